# revision 1
# baseline (speedup 1.0000x reference)
"""Trainium2 Bass kernel for nn_DecoderFusionBlock (VSS/Mamba decoder fusion block).

Two-pass SPMD over 8 cores (collectives unavailable under this runtime):
  pass 1: core c -> batch b=c//2, plane=c%2 (row-/col-major spatial order).
          Runs proj/LN/in_proj/dwconv/silu, then the selective scan for the
          plane's two directions (forward + reversed via reversed access
          patterns), producing the plane's merge partial Q (already rotated
          to row-major via data-driven masks), plus x (residual) and z (gate).
  host:   ym[b] = Q[2b] + Q[2b+1]  (the only cross-core reduction)
  pass 2: core c -> batch b=c//2: out-norm, gate, out_proj+residual,
          ConvBlock, final LN.
"""

import contextlib
import numpy as np

import concourse.bass as bass
import concourse.tile as tile
from concourse import bacc, mybir
from concourse.bass_utils import run_bass_kernel_spmd

f32 = mybir.dt.float32
f32r = mybir.dt.float32r
AF = mybir.ActivationFunctionType
OP_ = mybir.AluOpType

B_, H_, W_ = 4, 48, 48
L = H_ * W_
CIN, COUT = 192, 96
DIN, NST, RNK, KDIR = 192, 16, 6, 4
HID = 192
LC = 256
NCH = L // LC
NG = 4                           # n-values per scan group (4 groups of 4)
MMC = 512
EPS = 1e-5
DT0, DT1 = 128, 64
MM = [(s, min(MMC, L - s)) for s in range(0, L, MMC)]


def _fc(ap, c, lc=LC):
    return ap[:, c * lc:(c + 1) * lc]


def _rc(ap, c, lc=LC):
    hi = L - c * lc - 1
    lo = L - (c + 1) * lc - 1
    return ap[:, hi::-1] if lo < 0 else ap[:, hi:lo:-1]


def _swap_free(a):
    return bass.AP(tensor=a.tensor, offset=a.offset, ap=[a.ap[0], a.ap[2], a.ap[1]])


def _rep(a, n):
    return bass.AP(tensor=a.tensor, offset=a.offset, ap=[a.ap[0], [0, n], a.ap[1]])


def _twh(a):
    st = a.ap[1][0]
    return bass.AP(tensor=a.tensor, offset=a.offset,
                   ap=[a.ap[0], [st, 48], [48 * st, 48]])


def _pl3(a):
    st = a.ap[1][0]
    return bass.AP(tensor=a.tensor, offset=a.offset,
                   ap=[a.ap[0], [48 * st, 48], [st, 48]])


def _r(ap):
    # plain fp32 matmuls: the BIR verifier requires explicit rounding ops for
    # f32r operands, which would cost more than the 4x PE slowdown saves here
    return ap


# ---------------------------------------------------------------- pass 1
def build_nc1():
    nc = bacc.Bacc("TRN2", target_bir_lowering=False, debug=False, num_devices=8)
    din = {}

    def I(name, shape):
        din[name] = nc.dram_tensor(name, shape, f32, kind="ExternalInput")

    I("xc_t", [CIN, L]); I("projW", [CIN, COUT]); I("projb", [COUT, 1])
    I("W1", [COUT, 2 * DIN]); I("b1", [2 * DIN, 1])
    I("convW", [DIN, 9]); I("convb", [DIN, 1])
    I("xpw", [2, DIN, RNK + 2 * NST]); I("dtw", [2, RNK, DIN])
    I("dtb", [2, DIN, 1]); I("acoef", [2, DIN, NST]); I("dvec", [2, DIN, 1])
    I("sel16", [96, NST * 128]); I("mrow", [DIN, 1]); I("mcol", [DIN, 1])
    oq_d = nc.dram_tensor("oq", [DIN, L], f32, kind="ExternalOutput")
    ox_d = nc.dram_tensor("ox", [COUT, L], f32, kind="ExternalOutput")
    oz_d = nc.dram_tensor("oz", [DIN, L], f32, kind="ExternalOutput")

    ctx = contextlib.ExitStack()
    with tile.TileContext(nc) as tc, ctx:
        const = ctx.enter_context(tc.tile_pool(name="const", bufs=1))
        big = ctx.enter_context(tc.tile_pool(name="big", bufs=1))
        work = ctx.enter_context(tc.tile_pool(name="work", bufs=2))
        scn = ctx.enter_context(tc.tile_pool(name="scn", bufs=1))
        psM = ctx.enter_context(tc.tile_pool(name="psM", bufs=2, space="PSUM"))
        psB = ctx.enter_context(tc.tile_pool(name="psB", bufs=1, space="PSUM"))

        def load2(name, rows, cols):
            t0 = const.tile([DT0, cols], f32, tag=name + "0", name=name + "0")
            t1 = const.tile([DT1, cols], f32, tag=name + "1", name=name + "1")
            nc.sync.dma_start(t0[:], din[name][0:DT0])
            nc.sync.dma_start(t1[:], din[name][DT0:rows])
            return t0, t1

        projW0, projW1 = load2("projW", CIN, COUT)
        projb = const.tile([COUT, 1], f32)
        nc.sync.dma_start(projb[:], din["projb"][:])
        W1t = const.tile([COUT, 2 * DIN], f32)
        nc.sync.dma_start(W1t[:], din["W1"][:])
        b1x0 = const.tile([DT0, 1], f32); nc.sync.dma_start(b1x0[:], din["b1"][0:128])
        b1x1 = const.tile([DT1, 1], f32); nc.sync.dma_start(b1x1[:], din["b1"][128:192])
        b1z0 = const.tile([DT0, 1], f32); nc.sync.dma_start(b1z0[:], din["b1"][192:320])
        b1z1 = const.tile([DT1, 1], f32); nc.sync.dma_start(b1z1[:], din["b1"][320:384])
        convW0, convW1 = load2("convW", DIN, 9)
        convb0, convb1 = load2("convb", DIN, 1)
        sel16 = const.tile([96, NST * 128], f32)
        nc.sync.dma_start(sel16[:], din["sel16"][:])
        mrow0, mrow1 = load2("mrow", DIN, 1)
        mcol0, mcol1 = load2("mcol", DIN, 1)
        kw = []
        for k in range(2):
            xp0 = const.tile([DT0, RNK + 2 * NST], f32, name=f"xp{k}0")
            xp1 = const.tile([DT1, RNK + 2 * NST], f32, name=f"xp{k}1")
            nc.sync.dma_start(xp0[:], din["xpw"][k, 0:DT0])
            nc.sync.dma_start(xp1[:], din["xpw"][k, DT0:DIN])
            dtw = const.tile([RNK, DIN], f32, name=f"dtw{k}")
            nc.sync.dma_start(dtw[:], din["dtw"][k])
            dtb0 = const.tile([DT0, 1], f32, name=f"dtb{k}0")
            dtb1 = const.tile([DT1, 1], f32, name=f"dtb{k}1")
            nc.sync.dma_start(dtb0[:], din["dtb"][k, 0:DT0])
            nc.sync.dma_start(dtb1[:], din["dtb"][k, DT0:DIN])
            ac0 = const.tile([DT0, NST], f32, name=f"ac{k}0")
            ac1 = const.tile([DT1, NST], f32, name=f"ac{k}1")
            nc.sync.dma_start(ac0[:], din["acoef"][k, 0:DT0])
            nc.sync.dma_start(ac1[:], din["acoef"][k, DT0:DIN])
            dv0 = const.tile([DT0, 1], f32, name=f"dv{k}0")
            dv1 = const.tile([DT1, 1], f32, name=f"dv{k}1")
            nc.sync.dma_start(dv0[:], din["dvec"][k, 0:DT0])
            nc.sync.dma_start(dv1[:], din["dvec"][k, DT0:DIN])
            kw.append(dict(xp=(xp0, xp1), dtw=dtw, dtb=(dtb0, dtb1),
                           ac=(ac0, ac1), dv=(dv0, dv1)))

        ones128 = const.tile([128, 1], f32); nc.vector.memset(ones128[:], 1.0)
        onesrow = const.tile([1, 128], f32); nc.vector.memset(onesrow[:], 1.0)
        epsc = const.tile([1, 1], f32); nc.vector.memset(epsc[:], EPS)

        # ---- load + proj ----
        xc0 = big.tile([DT0, L], f32, tag="s0")
        xc1 = big.tile([DT1, L], f32, tag="s1")
        nc.sync.dma_start(xc0[:], din["xc_t"][0:DT0])
        nc.sync.dma_start(xc1[:], din["xc_t"][DT0:CIN])
        x_t = big.tile([COUT, L], f32, tag="s2")
        for (s, w) in MM:
            ps = psM.tile([128, MMC], f32, tag="mm", name="psproj")
            nc.tensor.matmul(ps[:COUT, :w], _r(projW0[:]), _r(xc0[:, s:s + w]),
                             start=True, stop=False)
            nc.tensor.matmul(ps[:COUT, :w], _r(projW1[:]), _r(xc1[:, s:s + w]),
                             start=False, stop=True)
            nc.scalar.activation(x_t[:, s:s + w], ps[:COUT, :w], AF.Identity,
                                 bias=projb[:])
        nc.sync.dma_start(ox_d[:], x_t[:])

        # ---- LN1 (over 96 channel partitions), fused stats+apply per chunk ----
        xn_t = big.tile([COUT, L], f32, tag="s0b")
        for (s, w) in MM:
            ps = psM.tile([128, MMC], f32, tag="mm", name="pss1")
            nc.tensor.matmul(ps[:1, :w], _r(ones128[:COUT]), _r(x_t[:, s:s + w]),
                             start=True, stop=True)
            mrw = work.tile([1, MMC], f32, tag="mrw", bufs=1)
            nc.scalar.activation(mrw[:, :w], ps[:1, :w], AF.Copy, scale=1.0 / COUT)
            sq = work.tile([128, MMC], f32, tag="sqc", bufs=1)
            nc.vector.tensor_tensor(out=sq[:COUT, :w], in0=x_t[:, s:s + w],
                                    in1=x_t[:, s:s + w], op=OP_.mult)
            ps2 = psM.tile([128, MMC], f32, tag="mm", name="pss2")
            nc.tensor.matmul(ps2[:1, :w], _r(ones128[:COUT]), _r(sq[:COUT, :w]),
                             start=True, stop=True)
            mq = work.tile([1, MMC], f32, tag="mq", bufs=1)
            nc.scalar.activation(mq[:, :w], ps2[:1, :w], AF.Copy, scale=1.0 / COUT)
            msq = work.tile([1, MMC], f32, tag="msq", bufs=1)
            nc.vector.tensor_tensor(out=msq[:, :w], in0=mrw[:, :w],
                                    in1=mrw[:, :w], op=OP_.mult)
            nc.vector.tensor_tensor(out=mq[:, :w], in0=mq[:, :w],
                                    in1=msq[:, :w], op=OP_.subtract)
            nc.scalar.activation(mq[:, :w], mq[:, :w], AF.Sqrt, bias=epsc[:])
            rsw = work.tile([1, MMC], f32, tag="rsw", bufs=1)
            nc.vector.reciprocal(rsw[:, :w], mq[:, :w])
            pm = psM.tile([128, MMC], f32, tag="mm", name="psbm")
            nc.tensor.matmul(pm[:, :w], _r(onesrow[:]), _r(mrw[:, :w]),
                             start=True, stop=True)
            pr = psM.tile([128, MMC], f32, tag="mm", name="psbr")
            nc.tensor.matmul(pr[:, :w], _r(onesrow[:]), _r(rsw[:, :w]),
                             start=True, stop=True)
            nc.vector.tensor_tensor(out=xn_t[:, s:s + w], in0=x_t[:, s:s + w],
                                    in1=pm[:COUT, :w], op=OP_.subtract)
            nc.vector.tensor_tensor(out=xn_t[:, s:s + w], in0=xn_t[:, s:s + w],
                                    in1=pr[:COUT, :w], op=OP_.mult)

        # ---- in_proj (x-part to xm tiles, z-part straight to DRAM) ----
        xm0 = big.tile([DT0, L], f32, tag="s3")
        xm1 = big.tile([DT1, L], f32, tag="s1b")
        for (s, w) in MM:
            for (coff, rows, bcol, dst, zoff) in (
                    (0, DT0, b1x0, xm0, None), (DT0, DT1, b1x1, xm1, None),
                    (DIN, DT0, b1z0, None, 0), (DIN + DT0, DT1, b1z1, None, DT0)):
                ps = psM.tile([128, MMC], f32, tag="mm", name="psip")
                nc.tensor.matmul(ps[:rows, :w], _r(W1t[:, coff:coff + rows]),
                                 _r(xn_t[:, s:s + w]), start=True, stop=True)
                if dst is not None:
                    nc.scalar.activation(dst[:, s:s + w], ps[:rows, :w], AF.Identity,
                                         bias=bcol[:])
                else:
                    zc = work.tile([128, MMC], f32, tag="zc", bufs=1)
                    nc.scalar.activation(zc[:rows, :w], ps[:rows, :w], AF.Identity,
                                         bias=bcol[:])
                    nc.sync.dma_start(oz_d[zoff:zoff + rows, s:s + w], zc[:rows, :w])

        # ---- depthwise conv + silu ----
        cv0 = big.tile([DT0, L], f32, tag="s2b")
        cv1 = big.tile([DT1, L], f32, tag="s4")
        for (src, wt, rows, out, eng) in ((xm0, convW0, DT0, cv0, nc.vector),
                                          (xm1, convW1, DT1, cv1, nc.gpsimd)):
            pad = work.tile([128, 50, 50], f32, tag="pad", bufs=1)
            eng.memset(pad[:rows], 0.0)
            eng.tensor_copy(out=pad[:rows, 1:49, 1:49], in_=_pl3(src[:]))
            ov = _pl3(out[:])
            onpool = eng is nc.gpsimd
            for j in range(9):
                dy, dx = divmod(j, 3)
                view = pad[:rows, dy:dy + 48, dx:dx + 48]
                if j == 0:
                    eng.tensor_scalar_mul(ov, view, wt[:, 0:1]) if eng is nc.gpsimd \
                        else nc.vector.tensor_scalar_mul(ov, view, wt[:, 0:1])
                else:
                    nc.vector.scalar_tensor_tensor(out=ov, in0=view, scalar=wt[:, j:j + 1],
                                                   in1=ov, op0=OP_.mult, op1=OP_.add)
        xs0 = big.tile([DT0, L], f32, tag="s3b")
        xs1 = big.tile([DT1, L], f32, tag="s5")
        nc.scalar.activation(xs0[:], cv0[:], AF.Silu, bias=convb0[:])
        nc.scalar.activation(xs1[:], cv1[:], AF.Silu, bias=convb1[:])
        # transposed plane, then data-driven select (both in place into xs)
        xt0 = big.tile([DT0, L], f32, tag="s2b", name="xt0")
        xt1 = big.tile([DT1, L], f32, tag="s4", name="xt1")
        nc.vector.tensor_copy(out=xt0[:], in_=_twh(xs0[:]))
        nc.gpsimd.tensor_copy(out=xt1[:], in_=_twh(xs1[:]))
        nc.vector.tensor_scalar_mul(xs0[:], xs0[:], mrow0[:])
        nc.vector.scalar_tensor_tensor(out=xs0[:], in0=xt0[:], scalar=mcol0[:],
                                       in1=xs0[:], op0=OP_.mult, op1=OP_.add)
        nc.gpsimd.tensor_scalar_mul(xs1[:], xs1[:], mrow1[:])
        xtm = big.tile([DT1, L], f32, tag="s4c", name="xtm")
        nc.gpsimd.tensor_scalar_mul(xtm[:], xt1[:], mcol1[:])
        nc.gpsimd.tensor_tensor(out=xs1[:], in0=xtm[:], in1=xs1[:], op=OP_.add)

        # ---- scan: k=0 forward, k=1 reversed ----
        P0 = big.tile([DT0, L], f32, tag="s6")
        P1 = big.tile([DT1, L], f32, tag="s7")
        for k in range(2):
            rev = (k == 1)
            W = kw[k]
            U96 = big.tile([96, L], f32, tag="u96", name=f"u96_{k}")
            for (s, w) in MM:
                ps = psM.tile([128, MMC], f32, tag="mm", name="psU")
                for (coff, ubase, m) in ((0, 0, RNK), (RNK, 32, NST),
                                         (RNK + NST, 64, NST)):
                    nc.tensor.matmul(ps[ubase:ubase + m, :w],
                                     _r(W["xp"][0][:, coff:coff + m]),
                                     _r(xs0[:, s:s + w]), start=True, stop=False)
                    nc.tensor.matmul(ps[ubase:ubase + m, :w],
                                     _r(W["xp"][1][:, coff:coff + m]),
                                     _r(xs1[:, s:s + w]), start=False, stop=True)
                for (ubase, m) in ((0, RNK), (32, NST), (64, NST)):
                    nc.vector.tensor_copy(out=U96[ubase:ubase + m, s:s + w],
                                          in_=ps[ubase:ubase + m, :w])

            hp = [[scn.tile([128, NG, 1], f32, tag=f"hp{dt}{h}", name=f"hp{dt}{h}")
                   for h in range(NST // NG)] for dt in range(2)]
            for dt in range(2):
                for h in range(NST // NG):
                    nc.vector.memset(hp[dt][h][:], 0.0)

            for c in range(NCH):
                uslice = (_rc(U96[0:RNK, :], c) if rev else _fc(U96[0:RNK, :], c))
                bsl = (_rc(U96[32:32 + NST, :], c) if rev else _fc(U96[32:32 + NST, :], c))
                csl = (_rc(U96[64:64 + NST, :], c) if rev else _fc(U96[64:64 + NST, :], c))
                # delta / dx chunks for both dtiles
                dcs, dxs = [], []
                for dt, (rows, dtbc, eng) in enumerate(((DT0, W["dtb"][0], nc.vector),
                                                        (DT1, W["dtb"][1], nc.gpsimd))):
                    ps = psM.tile([128, LC], f32, tag="mm", name="psdt")
                    nc.tensor.matmul(ps[:rows, :], _r(W["dtw"][:, dt * DT0:dt * DT0 + rows]),
                                     _r(uslice), start=True, stop=True)
                    dc = work.tile([128, LC], f32, tag=f"dc{dt}", name=f"dc{dt}", bufs=1)
                    nc.scalar.activation(dc[:rows], ps[:rows, :], AF.Exp,
                                         bias=dtbc[:])
                    nc.scalar.activation(dc[:rows], dc[:rows], AF.Ln, bias=1.0)
                    xsc = _rc((xs0 if dt == 0 else xs1)[:], c) if rev \
                        else _fc((xs0 if dt == 0 else xs1)[:], c)
                    dxc = work.tile([128, LC], f32, tag=f"dxc{dt}", name=f"dxc{dt}", bufs=1)
                    eng.tensor_tensor(out=dxc[:rows], in0=dc[:rows], in1=xsc, op=OP_.mult)
                    dcs.append(dc); dxs.append(dxc)

                yhs = []
                for h in range(NST // NG):
                    bt = psB.tile([128, NG, LC], f32, tag="bb", name="bb")
                    ct = psB.tile([128, NG, LC], f32, tag="cb", name="cb")
                    for q in range(NG):
                        n = h * NG + q
                        slb = _r(sel16[32:48, n * 128:(n + 1) * 128])
                        slc = _r(sel16[64:80, n * 128:(n + 1) * 128])
                        nc.tensor.matmul(bt[:, q, :], slb, _r(bsl), start=True, stop=True)
                        nc.tensor.matmul(ct[:, q, :], slc, _r(csl), start=True, stop=True)
                    for dt, (rows, eng) in enumerate(((DT0, nc.vector), (DT1, nc.vector))):
                        dA = scn.tile([128, NG, LC + 1], f32, tag=f"dA{dt}",
                                      name=f"dA{dt}", bufs=2)
                        dBu = scn.tile([128, NG, LC + 1], f32, tag=f"dBu{dt}",
                                       name=f"dBu{dt}")
                        Ht = scn.tile([128, NG, LC + 1], f32, tag=f"H{dt}",
                                      name=f"H{dt}")
                        nc.vector.memset(dA[:rows, :, 0:1], 0.0)
                        nc.gpsimd.tensor_copy(out=dBu[:rows, :, 0:1], in_=hp[dt][h][:rows])
                        for q in range(NG):
                            n = h * NG + q
                            nc.scalar.activation(dA[:rows, q, 1:], dcs[dt][:rows], AF.Exp,
                                                 scale=W["ac"][dt][:, n:n + 1])
                        nc.vector.tensor_tensor(out=dBu[:rows, :, 1:],
                                                in0=_rep(dxs[dt][:rows], NG),
                                                in1=bt[:rows], op=OP_.mult)
                        eng.tensor_tensor_scan(
                            out=Ht[:rows].rearrange("p a b -> p (a b)"),
                            data0=dA[:rows].rearrange("p a b -> p (a b)"),
                            data1=dBu[:rows].rearrange("p a b -> p (a b)"),
                            initial=0.0, op0=OP_.mult, op1=OP_.add)
                        nc.gpsimd.tensor_copy(out=hp[dt][h][:rows],
                                              in_=Ht[:rows, :, LC:LC + 1])
                        # G = H * C, into dA's buffer
                        nc.vector.tensor_tensor(out=dA[:rows, :, 1:],
                                                in0=Ht[:rows, :, 1:],
                                                in1=ct[:rows], op=OP_.mult)
                        if h == 0:
                            yh = scn.tile([128, LC], f32, tag=f"yh{dt}",
                                          name=f"yh{dt}")
                            nc.vector.tensor_reduce(
                                out=yh[:rows], in_=_swap_free(dA[:rows, :, 1:]),
                                axis=mybir.AxisListType.X, op=OP_.add)
                            yhs.append(yh)
                        else:
                            yh2 = scn.tile([128, LC], f32, tag=f"yh2{dt}",
                                           name=f"yh2{dt}")
                            nc.vector.tensor_reduce(
                                out=yh2[:rows], in_=_swap_free(dA[:rows, :, 1:]),
                                axis=mybir.AxisListType.X, op=OP_.add)
                            nc.gpsimd.tensor_tensor(out=yhs[dt][:rows],
                                                    in0=yhs[dt][:rows],
                                                    in1=yh2[:rows], op=OP_.add)
                # += D * xs ; accumulate into P
                for dt, (rows, Pt, xst, eng) in enumerate(
                        ((DT0, P0, xs0, nc.vector), (DT1, P1, xs1, nc.gpsimd))):
                    xsc = _rc(xst[:], c) if rev else _fc(xst[:], c)
                    dst_t = scn.tile([128, LC], f32, tag=f"dst{dt}", name=f"dst{dt}")
                    nc.gpsimd.tensor_scalar_mul(dst_t[:rows], xsc, W["dv"][dt][:])
                    nc.gpsimd.tensor_tensor(out=yhs[dt][:rows], in0=dst_t[:rows],
                                            in1=yhs[dt][:rows], op=OP_.add)
                    pdst = _rc(Pt[:], c) if rev else _fc(Pt[:], c)
                    if k == 0:
                        eng.tensor_copy(out=pdst, in_=yhs[dt][:rows])
                    else:
                        eng.tensor_tensor(out=pdst, in0=yhs[dt][:rows], in1=pdst,
                                          op=OP_.add)

        # ---- Q = mrow*P + mcol*transpose(P) ----
        Q0 = big.tile([DT0, L], f32, tag="s3b", name="Q0")
        Q1 = big.tile([DT1, L], f32, tag="s5", name="Q1")
        nc.vector.tensor_scalar_mul(Q0[:], _twh(P0[:]), mcol0[:])
        nc.vector.scalar_tensor_tensor(out=Q0[:], in0=P0[:], scalar=mrow0[:],
                                       in1=Q0[:], op0=OP_.mult, op1=OP_.add)
        nc.gpsimd.tensor_scalar_mul(Q1[:], _twh(P1[:]), mcol1[:])
        qtm = big.tile([DT1, L], f32, tag="s4c", name="qtm")
        nc.gpsimd.tensor_scalar_mul(qtm[:], P1[:], mrow1[:])
        nc.gpsimd.tensor_tensor(out=Q1[:], in0=qtm[:], in1=Q1[:], op=OP_.add)
        nc.sync.dma_start(oq_d[0:DT0], Q0[:])
        nc.sync.dma_start(oq_d[DT0:DIN], Q1[:])
    nc.compile()
    return nc


# ---------------------------------------------------------------- pass 2
def build_nc2():
    nc = bacc.Bacc("TRN2", target_bir_lowering=False, debug=False, num_devices=8)
    din = {}

    def I(name, shape):
        din[name] = nc.dram_tensor(name, shape, f32, kind="ExternalInput")

    I("ym", [DIN, L]); I("xin", [COUT, L]); I("zin", [DIN, L])
    I("OPm", [DIN, COUT]); I("OPB", [DIN, COUT])
    I("PW1", [COUT, HID]); I("g1", [HID, 1]); I("bb1", [HID, 1])
    I("CDW", [HID, 9]); I("g2", [HID, 1]); I("bb2", [HID, 1])
    I("PW2", [HID, COUT]); I("g3", [COUT, 1]); I("bb3", [COUT, 1])
    I("fw", [COUT, 1]); I("fb", [COUT, 1])
    out_d = nc.dram_tensor("o", [COUT, L], f32, kind="ExternalOutput")

    ctx = contextlib.ExitStack()
    with tile.TileContext(nc) as tc, ctx:
        const = ctx.enter_context(tc.tile_pool(name="const", bufs=1))
        big = ctx.enter_context(tc.tile_pool(name="big", bufs=1))
        work = ctx.enter_context(tc.tile_pool(name="work", bufs=2))
        psM = ctx.enter_context(tc.tile_pool(name="psM", bufs=2, space="PSUM"))

        def load2(name, rows, cols):
            t0 = const.tile([DT0, cols], f32, tag=name + "0", name=name + "0")
            t1 = const.tile([DT1, cols], f32, tag=name + "1", name=name + "1")
            nc.sync.dma_start(t0[:], din[name][0:DT0])
            nc.sync.dma_start(t1[:], din[name][DT0:rows])
            return t0, t1

        def load1(name, rows):
            t = const.tile([rows, 1], f32, tag=name, name=name)
            nc.sync.dma_start(t[:], din[name][:])
            return t

        OP0, OP1 = load2("OPm", DIN, COUT)
        OPB0, OPB1 = load2("OPB", DIN, COUT)
        PW1t = const.tile([COUT, HID], f32)
        nc.sync.dma_start(PW1t[:], din["PW1"][:])
        g1c0, g1c1 = load2("g1", HID, 1)
        bb1c0, bb1c1 = load2("bb1", HID, 1)
        CDW0, CDW1 = load2("CDW", HID, 9)
        g2c0, g2c1 = load2("g2", HID, 1)
        bb2c0, bb2c1 = load2("bb2", HID, 1)
        PW20, PW21 = load2("PW2", HID, COUT)
        g3c = load1("g3", COUT); bb3c = load1("bb3", COUT)
        fwc = load1("fw", COUT); fbc = load1("fb", COUT)
        ones128 = const.tile([128, 1], f32); nc.vector.memset(ones128[:], 1.0)
        onesrow = const.tile([1, 128], f32); nc.vector.memset(onesrow[:], 1.0)
        epsc = const.tile([1, 1], f32); nc.vector.memset(epsc[:], EPS)

        ym0 = big.tile([DT0, L], f32, tag="s0")
        ym1 = big.tile([DT1, L], f32, tag="s1")
        nc.sync.dma_start(ym0[:], din["ym"][0:DT0])
        nc.sync.dma_start(ym1[:], din["ym"][DT0:DIN])
        xres = big.tile([COUT, L], f32, tag="s2")
        nc.sync.dma_start(xres[:], din["xin"][:])

        # stats over 192 partitions + per-chunk post chain
        mean_r = big.tile([1, L], f32, tag="mean")
        ms_r = big.tile([1, L], f32, tag="ms")
        for (s, w) in MM:
            ps = psM.tile([128, MMC], f32, tag="mm", name="pso1")
            nc.tensor.matmul(ps[:1, :w], _r(ones128[:]), _r(ym0[:, s:s + w]),
                             start=True, stop=False)
            nc.tensor.matmul(ps[:1, :w], _r(ones128[:DT1]), _r(ym1[:, s:s + w]),
                             start=False, stop=True)
            nc.scalar.activation(mean_r[:, s:s + w], ps[:1, :w], AF.Copy,
                                 scale=1.0 / DIN)
            ps2 = psM.tile([128, MMC], f32, tag="mm", name="pso2")
            for i, (t, rows) in enumerate(((ym0, DT0), (ym1, DT1))):
                sq = work.tile([128, MMC], f32, tag="sqc", bufs=1)
                nc.vector.tensor_tensor(out=sq[:rows, :w], in0=t[:, s:s + w],
                                        in1=t[:, s:s + w], op=OP_.mult)
                nc.tensor.matmul(ps2[:1, :w], _r(ones128[:rows]), _r(sq[:rows, :w]),
                                 start=(i == 0), stop=(i == 1))
            nc.scalar.activation(ms_r[:, s:s + w], ps2[:1, :w], AF.Copy,
                                 scale=1.0 / DIN)

        x2 = big.tile([COUT, L], f32, tag="s3")
        for (s, w) in MM:
            mq = work.tile([1, MMC], f32, tag="mq", bufs=1)
            nc.vector.tensor_tensor(out=mq[:, :w], in0=mean_r[:, s:s + w],
                                    in1=mean_r[:, s:s + w], op=OP_.mult)
            nc.vector.tensor_tensor(out=mq[:, :w], in0=ms_r[:, s:s + w],
                                    in1=mq[:, :w], op=OP_.subtract)
            nc.scalar.activation(mq[:, :w], mq[:, :w], AF.Sqrt, bias=epsc[:])
            rsw = work.tile([1, MMC], f32, tag="rsw", bufs=1)
            nc.vector.reciprocal(rsw[:, :w], mq[:, :w])
            pm = psM.tile([128, MMC], f32, tag="mm", name="psm")
            nc.tensor.matmul(pm[:, :w], _r(onesrow[:]), _r(mean_r[:, s:s + w]),
                             start=True, stop=True)
            pr = psM.tile([128, MMC], f32, tag="mm", name="psr")
            nc.tensor.matmul(pr[:, :w], _r(onesrow[:]), _r(rsw[:, :w]),
                             start=True, stop=True)
            po = psM.tile([128, MMC], f32, tag="mm", name="pso")
            for i, (t, rows, zoff) in enumerate(((ym0, DT0, 0), (ym1, DT1, DT0))):
                yn = work.tile([128, MMC], f32, tag=f"yn{i}", name=f"yn{i}")
                nc.vector.tensor_tensor(out=yn[:rows, :w], in0=t[:, s:s + w],
                                        in1=pm[:rows, :w], op=OP_.subtract)
                nc.vector.tensor_tensor(out=yn[:rows, :w], in0=yn[:rows, :w],
                                        in1=pr[:rows, :w], op=OP_.mult)
                zc = work.tile([128, MMC], f32, tag=f"zc{i}", name=f"zc{i}")
                nc.sync.dma_start(zc[:rows, :w], din["zin"][zoff:zoff + rows, s:s + w])
                gc = work.tile([128, MMC], f32, tag=f"gc{i}", name=f"gc{i}")
                nc.scalar.activation(gc[:rows, :w], zc[:rows, :w], AF.Silu)
                nc.vector.tensor_tensor(out=yn[:rows, :w], in0=yn[:rows, :w],
                                        in1=gc[:rows, :w], op=OP_.mult)
                OPt = OP0 if i == 0 else OP1
                OPBt = OPB0 if i == 0 else OPB1
                nc.tensor.matmul(po[:COUT, :w], _r(OPt[:]), _r(yn[:rows, :w]),
                                 start=(i == 0), stop=False)
                nc.tensor.matmul(po[:COUT, :w], _r(OPBt[:]), _r(gc[:rows, :w]),
                                 start=False, stop=(i == 1))
            nc.vector.tensor_tensor(out=x2[:, s:s + w], in0=po[:COUT, :w],
                                    in1=xres[:, s:s + w], op=OP_.add)

        # ConvBlock
        t0 = big.tile([DT0, L], f32, tag="s4")
        t1 = big.tile([DT1, L], f32, tag="s5")
        for (s, w) in MM:
            for (dst, coff, rows, gc_, bc_) in ((t0, 0, DT0, g1c0, bb1c0),
                                                (t1, DT0, DT1, g1c1, bb1c1)):
                ps = psM.tile([128, MMC], f32, tag="mm", name="psp1")
                nc.tensor.matmul(ps[:rows, :w], _r(PW1t[:, coff:coff + rows]),
                                 _r(x2[:, s:s + w]), start=True, stop=True)
                nc.scalar.activation(dst[:, s:s + w], ps[:rows, :w], AF.Gelu,
                                     bias=bc_[:], scale=gc_[:])
        u0 = big.tile([DT0, L], f32, tag="s0b")
        u1 = big.tile([DT1, L], f32, tag="s1b")
        for (src, wt, rows, out, eng) in ((t0, CDW0, DT0, u0, nc.vector),
                                          (t1, CDW1, DT1, u1, nc.gpsimd)):
            pad = work.tile([128, 50, 50], f32, tag="pad", bufs=1)
            eng.memset(pad[:rows], 0.0)
            eng.tensor_copy(out=pad[:rows, 1:49, 1:49], in_=_pl3(src[:]))
            ov = _pl3(out[:])
            onpool = eng is nc.gpsimd
            for j in range(9):
                dy, dx = divmod(j, 3)
                view = pad[:rows, dy:dy + 48, dx:dx + 48]
                if j == 0:
                    eng.tensor_scalar_mul(ov, view, wt[:, 0:1]) if eng is nc.gpsimd \
                        else nc.vector.tensor_scalar_mul(ov, view, wt[:, 0:1])
                else:
                    nc.vector.scalar_tensor_tensor(out=ov, in0=view, scalar=wt[:, j:j + 1],
                                                   in1=ov, op0=OP_.mult, op1=OP_.add)
        x3 = big.tile([COUT, L], f32, tag="s2b")
        for (s, w) in MM:
            ps = psM.tile([128, MMC], f32, tag="mm", name="psp2")
            for i, (ut, rows, gc_, bc_) in enumerate(((u0, DT0, g2c0, bb2c0),
                                                      (u1, DT1, g2c1, bb2c1))):
                vc = work.tile([128, MMC], f32, tag=f"vc{i}", name=f"vc{i}")
                nc.scalar.activation(vc[:rows, :w], ut[:, s:s + w], AF.Gelu,
                                     bias=bc_[:], scale=gc_[:])
                PWt = PW20 if i == 0 else PW21
                nc.tensor.matmul(ps[:COUT, :w], _r(PWt[:]), _r(vc[:rows, :w]),
                                 start=(i == 0), stop=(i == 1))
            cbt = work.tile([128, MMC], f32, tag="cbt", bufs=1)
            nc.scalar.activation(cbt[:COUT, :w], ps[:COUT, :w], AF.Identity,
                                 bias=bb3c[:], scale=g3c[:])
            nc.vector.tensor_tensor(out=x3[:, s:s + w], in0=cbt[:COUT, :w],
                                    in1=x2[:, s:s + w], op=OP_.add)

        # final LN
        mean2 = big.tile([1, L], f32, tag="mean2")
        ms2 = big.tile([1, L], f32, tag="ms2")
        for (s, w) in MM:
            ps = psM.tile([128, MMC], f32, tag="mm", name="psf1")
            nc.tensor.matmul(ps[:1, :w], _r(ones128[:COUT]), _r(x3[:, s:s + w]),
                             start=True, stop=True)
            nc.scalar.activation(mean2[:, s:s + w], ps[:1, :w], AF.Copy,
                                 scale=1.0 / COUT)
            sq = work.tile([128, MMC], f32, tag="sqc", bufs=1)
            nc.vector.tensor_tensor(out=sq[:COUT, :w], in0=x3[:, s:s + w],
                                    in1=x3[:, s:s + w], op=OP_.mult)
            ps2 = psM.tile([128, MMC], f32, tag="mm", name="psf2")
            nc.tensor.matmul(ps2[:1, :w], _r(ones128[:COUT]), _r(sq[:COUT, :w]),
                             start=True, stop=True)
            nc.scalar.activation(ms2[:, s:s + w], ps2[:1, :w], AF.Copy,
                                 scale=1.0 / COUT)
        for (s, w) in MM:
            mq = work.tile([1, MMC], f32, tag="mq", bufs=1)
            nc.vector.tensor_tensor(out=mq[:, :w], in0=mean2[:, s:s + w],
                                    in1=mean2[:, s:s + w], op=OP_.mult)
            nc.vector.tensor_tensor(out=mq[:, :w], in0=ms2[:, s:s + w],
                                    in1=mq[:, :w], op=OP_.subtract)
            nc.scalar.activation(mq[:, :w], mq[:, :w], AF.Sqrt, bias=epsc[:])
            rsw = work.tile([1, MMC], f32, tag="rsw", bufs=1)
            nc.vector.reciprocal(rsw[:, :w], mq[:, :w])
            pm = psM.tile([128, MMC], f32, tag="mm", name="psfm")
            nc.tensor.matmul(pm[:, :w], _r(onesrow[:]), _r(mean2[:, s:s + w]),
                             start=True, stop=True)
            pr = psM.tile([128, MMC], f32, tag="mm", name="psfr")
            nc.tensor.matmul(pr[:, :w], _r(onesrow[:]), _r(rsw[:, :w]),
                             start=True, stop=True)
            oc = work.tile([128, MMC], f32, tag="oc", bufs=1)
            nc.vector.tensor_tensor(out=oc[:COUT, :w], in0=x3[:, s:s + w],
                                    in1=pm[:COUT, :w], op=OP_.subtract)
            nc.vector.tensor_tensor(out=oc[:COUT, :w], in0=oc[:COUT, :w],
                                    in1=pr[:COUT, :w], op=OP_.mult)
            nc.vector.tensor_scalar(out=oc[:COUT, :w], in0=oc[:COUT, :w],
                                    scalar1=fwc[:], scalar2=fbc[:],
                                    op0=OP_.mult, op1=OP_.add)
            nc.sync.dma_start(out_d[:, s:s + w], oc[:COUT, :w])
    nc.compile()
    return nc


_NC1, _NC2 = None, None


def _get_ncs():
    global _NC1, _NC2
    if _NC1 is None:
        _NC1 = build_nc1()
        _NC2 = build_nc2()
    return _NC1, _NC2


def prep_pass1(ip):
    W1 = (np.diag(ip["ln1_w"]) @ ip["in_proj_W"]).astype(np.float32)
    b1 = (ip["ln1_b"] @ ip["in_proj_W"] + ip["in_proj_b"]).astype(np.float32)
    A = (-np.exp(ip["A_logs"].astype(np.float64))).astype(np.float32).reshape(KDIR, DIN, NST)
    Ds = ip["Ds"].reshape(KDIR, DIN)
    sel16 = np.zeros((96, NST * 128), np.float32)
    for n in range(NST):
        for base in (0, 32, 64):
            sel16[base + n, n * 128:(n + 1) * 128] = 1.0
    col = lambda v: np.ascontiguousarray(v.reshape(-1, 1), dtype=np.float32)
    base = dict(projW=ip["proj_W"], projb=col(ip["proj_b"]), W1=W1, b1=col(b1),
                convW=np.ascontiguousarray(ip["conv_W"].reshape(DIN, 9)),
                convb=col(ip["conv_b"]), sel16=sel16)
    maps = []
    for c in range(8):
        b, plane = c // 2, c % 2
        ks = [plane, plane + 2]
        m = dict(base)
        m["xc_t"] = np.ascontiguousarray(ip["x_cat"][b].reshape(L, CIN).T)
        m["xpw"] = np.ascontiguousarray(np.stack([ip["x_proj_W"][k].T for k in ks]))
        m["dtw"] = np.ascontiguousarray(np.stack([ip["dt_W"][k].T for k in ks]))
        m["dtb"] = np.ascontiguousarray(np.stack([col(ip["dt_b"][k]) for k in ks]))
        m["acoef"] = np.ascontiguousarray(np.stack([A[k] for k in ks]))
        m["dvec"] = np.ascontiguousarray(np.stack([col(Ds[k]) for k in ks]))
        m["mrow"] = np.full((DIN, 1), 1.0 - plane, np.float32)
        m["mcol"] = np.full((DIN, 1), float(plane), np.float32)
        maps.append(m)
    return maps


def prep_pass2(ip, res1):
    OPm = (np.diag(ip["out_norm_w"]) @ ip["out_proj_W"]).astype(np.float32)
    OPB = (np.diag(ip["out_norm_b"]) @ ip["out_proj_W"]).astype(np.float32)
    col = lambda v: np.ascontiguousarray(v.reshape(-1, 1), dtype=np.float32)
    base = dict(OPm=OPm, OPB=OPB,
                PW1=np.ascontiguousarray(ip["cb_pw1_W"][:, :, 0, 0].T),
                g1=col(ip["cb_bn1_g"]), bb1=col(ip["cb_bn1_b"]),
                CDW=np.ascontiguousarray(ip["cb_dw_W"].reshape(HID, 9)),
                g2=col(ip["cb_bn2_g"]), bb2=col(ip["cb_bn2_b"]),
                PW2=np.ascontiguousarray(ip["cb_pw2_W"][:, :, 0, 0].T),
                g3=col(ip["cb_bn3_g"]), bb3=col(ip["cb_bn3_b"]),
                fw=col(ip["norm_w"]), fb=col(ip["norm_b"]))
    maps = []
    for c in range(8):
        b = c // 2
        m = dict(base)
        m["ym"] = res1[2 * b]["oq"] + res1[2 * b + 1]["oq"]
        m["xin"] = res1[2 * b]["ox"]
        m["zin"] = res1[2 * b]["oz"]
        maps.append(m)
    return maps


def kernel(**inputs):
    ip = {k: np.asarray(v, np.float32) for k, v in inputs.items()}
    nc1, nc2 = _get_ncs()
    res1 = run_bass_kernel_spmd(nc1, prep_pass1(ip), list(range(8))).results
    res2 = run_bass_kernel_spmd(nc2, prep_pass2(ip, res1), list(range(8))).results
    outs = [res2[2 * b]["o"].T.reshape(H_, W_, COUT) for b in range(B_)]
    return np.stack(outs).astype(np.float32)



# revision 25
# speedup vs baseline: 1.6814x; 1.6814x over previous
"""Trainium2 Bass kernel for nn_DecoderFusionBlock (VSS/Mamba decoder fusion block).

Two-pass SPMD over 8 cores:
  pass 1: core c -> batch b=c//2, plane=c%2 (row-/col-major spatial order).
          proj/LN/in_proj (f32r / bf16 matmuls), depthwise conv via PE diag
          matmuls, then the selective scan for the plane's two directions.
          bf16 data path with fp32 scan state; B/C broadcast to all channel
          partitions via a DRAM-staged broadcast DMA so the big elementwise
          multiplies run in the DVE 2x (2-byte) mode; the n-state reduction
          runs on the PE as identity-weight matmul accumulation in PSUM.
  host:   ym[b] = Q[2b] + Q[2b+1]  (the only cross-core reduction)
  pass 2: core c -> batch b=c//2: out-norm, gate, out_proj+residual,
          ConvBlock (conv again via PE), final LayerNorm.
"""

import contextlib
import numpy as np

import concourse.bass as bass
import concourse.tile as tile
from concourse import bacc, mybir
from concourse.bass_utils import run_bass_kernel_spmd

f32 = mybir.dt.float32
f32r = mybir.dt.float32r
bf16 = mybir.dt.bfloat16
AF = mybir.ActivationFunctionType
OP_ = mybir.AluOpType

B_, H_, W_ = 4, 48, 48
L = H_ * W_
CIN, COUT = 192, 96
DIN, NST, RNK, KDIR = 192, 16, 6, 4
HID = 192
EPS = 1e-5
DT0, DT1 = 128, 64
MMC = 512
MM = [(s, min(MMC, L - s)) for s in range(0, L, MMC)]
LC = 256
SC = [(i * LC, LC) for i in range(L // LC)]
CROWS = [(0, 10), (10, 10), (20, 10), (30, 10), (40, 8)]


def _rev(ap, s, w):
    hi = L - 1 - s
    lo = hi - w
    return ap[:, hi::-1] if lo < 0 else ap[:, hi:lo:-1]


def _sl(ap, k, s, w):
    return ap[:, s:s + w] if k == 0 else _rev(ap, s, w)


def _rep(a, n):
    return bass.AP(tensor=a.tensor, offset=a.offset, ap=[a.ap[0], [0, n], a.ap[1]])


def _twh(a):
    st = a.ap[1][0]
    return bass.AP(tensor=a.tensor, offset=a.offset,
                   ap=[a.ap[0], [st, 48], [48 * st, 48]])


def _pl3(a):
    st = a.ap[1][0]
    return bass.AP(tensor=a.tensor, offset=a.offset,
                   ap=[a.ap[0], [48 * st, 48], [st, 48]])


# ---------------------------------------------------------------- pass 1
def build_nc1():
    nc = bacc.Bacc("TRN2", target_bir_lowering=False, debug=False, num_devices=8)
    din = {}

    def I(name, shape, dt=f32):
        din[name] = nc.dram_tensor(name, shape, dt, kind="ExternalInput")

    I("xc_t", [CIN, L], f32r)
    I("projW", [CIN, COUT], f32r); I("projb", [COUT, 1])
    I("W1", [COUT, 2 * DIN], bf16); I("b1", [2 * DIN, 1])
    I("cdiag0", [DT0, 9, DT0], bf16); I("cdiag1", [DT1, 9, DT1], bf16)
    I("convb", [DIN, 1])
    I("eye", [128, 128], bf16)
    I("xpw", [2, DIN, RNK + 2 * NST], bf16)
    I("dtw", [2, RNK, DIN], bf16)
    I("dtb", [2, DIN, 1]); I("acoef", [2, DIN, NST]); I("dsum", [DIN, 1])
    I("mrow", [DIN, 1]); I("mcol", [DIN, 1])
    oq_d = nc.dram_tensor("oq", [DIN, L], bf16, kind="ExternalOutput")
    ox_d = nc.dram_tensor("ox", [COUT, L], f32, kind="ExternalOutput")
    oz_d = nc.dram_tensor("oz", [DIN, L], bf16, kind="ExternalOutput")
    bcd = nc.dram_tensor("BCd", [2, 32, L], bf16, kind="Internal")

    ctx = contextlib.ExitStack()
    with tile.TileContext(nc) as tc, ctx:
        const = ctx.enter_context(tc.tile_pool(name="const", bufs=1))
        big = ctx.enter_context(tc.tile_pool(name="big", bufs=1))
        work = ctx.enter_context(tc.tile_pool(name="work", bufs=2))
        scn = ctx.enter_context(tc.tile_pool(name="scn", bufs=1))
        psM = ctx.enter_context(tc.tile_pool(name="psM", bufs=2, space="PSUM"))
        psY = ctx.enter_context(tc.tile_pool(name="psY", bufs=2, space="PSUM"))

        def load2(name, rows, cols, dt=f32):
            t0 = const.tile([DT0, cols], dt, tag=name + "0", name=name + "0")
            t1 = const.tile([DT1, cols], dt, tag=name + "1", name=name + "1")
            nc.sync.dma_start(t0[:], din[name][0:DT0])
            nc.sync.dma_start(t1[:], din[name][DT0:rows])
            return t0, t1

        projW0 = const.tile([DT0, COUT], f32r)
        projW1 = const.tile([DT1, COUT], f32r)
        nc.sync.dma_start(projW0[:], din["projW"][0:DT0])
        nc.sync.dma_start(projW1[:], din["projW"][DT0:CIN])
        projb = const.tile([COUT, 1], f32)
        nc.sync.dma_start(projb[:], din["projb"][:])
        W1t = const.tile([COUT, 2 * DIN], bf16)
        nc.sync.dma_start(W1t[:], din["W1"][:])
        b1x0 = const.tile([DT0, 1], f32); nc.sync.dma_start(b1x0[:], din["b1"][0:128])
        b1x1 = const.tile([DT1, 1], f32); nc.sync.dma_start(b1x1[:], din["b1"][128:192])
        b1z0 = const.tile([DT0, 1], f32); nc.sync.dma_start(b1z0[:], din["b1"][192:320])
        b1z1 = const.tile([DT1, 1], f32); nc.sync.dma_start(b1z1[:], din["b1"][320:384])
        cdiag0 = const.tile([DT0, 9, DT0], bf16)
        nc.sync.dma_start(cdiag0[:], din["cdiag0"][:])
        cdiag1 = const.tile([DT1, 9, DT1], bf16)
        nc.sync.dma_start(cdiag1[:], din["cdiag1"][:])
        convb0, convb1 = load2("convb", DIN, 1)
        eye = const.tile([128, 128], bf16)
        nc.sync.dma_start(eye[:], din["eye"][:])
        dsum0, dsum1 = load2("dsum", DIN, 1)
        mrow0, mrow1 = load2("mrow", DIN, 1)
        mcol0, mcol1 = load2("mcol", DIN, 1)
        kw = []
        for k in range(2):
            xp0 = const.tile([DT0, RNK + 2 * NST], bf16, name=f"xp{k}0")
            xp1 = const.tile([DT1, RNK + 2 * NST], bf16, name=f"xp{k}1")
            nc.sync.dma_start(xp0[:], din["xpw"][k, 0:DT0])
            nc.sync.dma_start(xp1[:], din["xpw"][k, DT0:DIN])
            dtw = const.tile([38, DIN], bf16, tag="dtwm", name=f"dtw{k}",
                             bufs=1) if k == 0 else kw[0]["dtwt"]
            nc.sync.dma_start(dtw[k * 32:k * 32 + RNK], din["dtw"][k])
            dtb0 = const.tile([DT0, 1], f32, name=f"dtb{k}0")
            dtb1 = const.tile([DT1, 1], f32, name=f"dtb{k}1")
            nc.sync.dma_start(dtb0[:], din["dtb"][k, 0:DT0])
            nc.sync.dma_start(dtb1[:], din["dtb"][k, DT0:DIN])
            ac0 = const.tile([DT0, NST], f32, name=f"ac{k}0")
            ac1 = const.tile([DT1, NST], f32, name=f"ac{k}1")
            nc.sync.dma_start(ac0[:], din["acoef"][k, 0:DT0])
            nc.sync.dma_start(ac1[:], din["acoef"][k, DT0:DIN])
            kw.append(dict(xp=(xp0, xp1), dtwt=dtw,
                           dtw=dtw[k * 32:k * 32 + RNK], dtb=(dtb0, dtb1),
                           ac=(ac0, ac1)))

        ones128 = const.tile([128, 1], f32); nc.vector.memset(ones128[:], 1.0)
        onesrow = const.tile([1, 128], bf16); nc.vector.memset(onesrow[:], 1.0)
        epsc = const.tile([1, 1], f32); nc.vector.memset(epsc[:], EPS)

        # ---- load + proj (f32r matmuls, x_t kept fp32 for residual) ----
        xc0 = big.tile([DT0, L], f32r, tag="xc0")
        xc1 = big.tile([DT1, L], f32r, tag="xc1")
        nc.sync.dma_start(xc0[:], din["xc_t"][0:DT0])
        nc.sync.dma_start(xc1[:], din["xc_t"][DT0:CIN])
        x_t = big.tile([COUT, L], f32, tag="x_t")
        for (s, w) in MM:
            ps = psM.tile([128, MMC], f32, tag="mm", name="psproj")
            nc.tensor.matmul(ps[:COUT, :w], projW0[:], xc0[:, s:s + w],
                             start=True, stop=False)
            nc.tensor.matmul(ps[:COUT, :w], projW1[:], xc1[:, s:s + w],
                             start=False, stop=True)
            nc.scalar.activation(x_t[:, s:s + w], ps[:COUT, :w], AF.Identity,
                                 bias=projb[:])
        nc.sync.dma_start(ox_d[:], x_t[:])

        # ---- LN1 (Copy + Sqrt share the act-table phase) -> xn bf16 ----
        xn_t = big.tile([COUT, L], bf16, tag="xn")
        for (s, w) in MM:
            ps1 = psM.tile([128, MMC], f32, tag="mm", name="pss1")
            nc.tensor.matmul(ps1[:1, :w], ones128[:COUT], x_t[:, s:s + w],
                             start=True, stop=True)
            mrw = work.tile([1, MMC], bf16, tag="mrw", bufs=1)
            nc.scalar.activation(mrw[:, :w], ps1[:1, :w], AF.Copy, scale=1.0 / COUT)
            sq = work.tile([128, MMC], f32, tag="sqc", bufs=1)
            nc.vector.tensor_tensor(out=sq[:COUT, :w], in0=x_t[:, s:s + w],
                                    in1=x_t[:, s:s + w], op=OP_.mult)
            ps2 = psM.tile([128, MMC], f32, tag="mm", name="pss2")
            nc.tensor.matmul(ps2[:1, :w], ones128[:COUT], sq[:COUT, :w],
                             start=True, stop=True)
            mq = work.tile([1, MMC], f32, tag="mq", bufs=1)
            nc.scalar.activation(mq[:, :w], ps2[:1, :w], AF.Copy, scale=1.0 / COUT)
            msq = work.tile([1, MMC], f32, tag="msq", bufs=1)
            nc.vector.tensor_tensor(out=msq[:, :w], in0=mrw[:, :w],
                                    in1=mrw[:, :w], op=OP_.mult)
            nc.vector.tensor_tensor(out=mq[:, :w], in0=mq[:, :w],
                                    in1=msq[:, :w], op=OP_.subtract)
            nc.scalar.activation(mq[:, :w], mq[:, :w], AF.Sqrt, bias=epsc[:])
            rsw = work.tile([1, MMC], bf16, tag="rsw", bufs=1)
            with nc.allow_low_precision(reason="bf16 1/std is well conditioned"):
                nc.vector.reciprocal(rsw[:, :w], mq[:, :w])
            pm = psM.tile([128, MMC], f32, tag="mm", name="psbm")
            nc.tensor.matmul(pm[:, :w], onesrow[:], mrw[:, :w],
                             start=True, stop=True)
            pr = psM.tile([128, MMC], f32, tag="mm", name="psbr")
            nc.tensor.matmul(pr[:, :w], onesrow[:], rsw[:, :w],
                             start=True, stop=True)
            xn_ = work.tile([128, MMC], bf16, tag="xn_", bufs=1)
            nc.vector.tensor_tensor(out=xn_[:COUT, :w], in0=x_t[:, s:s + w],
                                    in1=pm[:COUT, :w], op=OP_.subtract)
            nc.vector.tensor_tensor(out=xn_t[:, s:s + w], in0=xn_[:COUT, :w],
                                    in1=pr[:COUT, :w], op=OP_.mult)

        # ---- in_proj (xm tiles bf16; z silu'd -> DRAM bf16) ----
        xm0 = big.tile([DT0, L], bf16, tag="xm0")
        xm1 = big.tile([DT1, L], bf16, tag="xm1")
        for (s, w) in MM:
            for (coff, rows, bcol, dst, zoff) in (
                    (0, DT0, b1x0, xm0, None), (DT0, DT1, b1x1, xm1, None),
                    (DIN, DT0, b1z0, None, 0), (DIN + DT0, DT1, b1z1, None, DT0)):
                psi = psM.tile([128, MMC], f32, tag="mm", name="psip")
                nc.tensor.matmul(psi[:rows, :w], W1t[:, coff:coff + rows],
                                 xn_t[:, s:s + w], start=True, stop=True)
                if dst is not None:
                    nc.scalar.activation(dst[:, s:s + w], psi[:rows, :w],
                                         AF.Identity, bias=bcol[:])
                else:
                    zc = work.tile([128, MMC], bf16, tag="zc", bufs=1)
                    nc.scalar.activation(zc[:rows, :w], psi[:rows, :w], AF.Silu,
                                         bias=bcol[:])
                    nc.sync.dma_start(oz_d[zoff:zoff + rows, s:s + w], zc[:rows, :w])

        # ---- depthwise conv via PE diag matmuls + fused SiLU ----
        xs0 = big.tile([DT0, L], bf16, tag="xs0")
        xs1 = big.tile([DT1, L], bf16, tag="xs1")
        for (src, cd, rows, out, bcol) in ((xm0, cdiag0, DT0, xs0, convb0),
                                           (xm1, cdiag1, DT1, xs1, convb1)):
            pad = work.tile([128, 50, 50], bf16, tag="pad", bufs=1)
            nc.vector.memset(pad[:rows], 0.0)
            nc.vector.tensor_copy(out=pad[:rows, 1:49, 1:49], in_=_pl3(src[:]))
            for (y0, ny) in CROWS:
                pc = psM.tile([128, MMC], f32, tag="mm", name="pscv")
                for j in range(9):
                    dy, dx = divmod(j, 3)
                    view = pad[:rows, y0 + dy:y0 + dy + ny, dx:dx + 48]
                    nc.tensor.matmul(pc[:rows, :ny * 48], cd[:, j], view,
                                     start=(j == 0), stop=(j == 8))
                nc.scalar.activation(out[:, y0 * 48:(y0 + ny) * 48],
                                     pc[:rows, :ny * 48], AF.Silu, bias=bcol[:])

        # ---- plane transform: xs = mrow*xs + mcol*T(xs) ----
        xt0 = big.tile([DT0, L], bf16, tag="xm0", name="xt0")
        xt1 = big.tile([DT1, L], bf16, tag="xm1", name="xt1")
        nc.vector.tensor_copy(out=xt0[:], in_=_twh(xs0[:]))
        nc.gpsimd.tensor_copy(out=xt1[:], in_=_twh(xs1[:]))
        nc.vector.tensor_scalar_mul(xs0[:], xs0[:], mrow0[:])
        nc.vector.scalar_tensor_tensor(out=xs0[:], in0=xt0[:], scalar=mcol0[:],
                                       in1=xs0[:], op0=OP_.mult, op1=OP_.add)
        nc.gpsimd.tensor_scalar_mul(xs1[:], xs1[:], mrow1[:])
        xtm = big.tile([DT1, L], bf16, tag="xt1m")
        nc.gpsimd.tensor_scalar_mul(xtm[:], xt1[:], mcol1[:])
        nc.gpsimd.tensor_tensor(out=xs1[:], in0=xtm[:], in1=xs1[:], op=OP_.add)

        # ---- U96: x_dbl for both k; B/C staged to DRAM bf16 ----
        u96m = big.tile([38, L], bf16, tag="u96r")
        u96rs = (u96m[0:RNK], u96m[32:38])
        for k in range(2):
            W = kw[k]
            rb = k * 32          # rank section base: 0 (k0) / 32 (k1)
            bb = rb + 32
            cb = 64 if k == 0 else 0
            for (s, w) in MM:
                ps = psM.tile([128, MMC], f32, tag="mm", name="psU")
                for (coff, ubase, m) in ((0, rb, RNK), (RNK, bb, NST),
                                         (RNK + NST, cb, NST)):
                    nc.tensor.matmul(ps[ubase:ubase + m, :w],
                                     W["xp"][0][:, coff:coff + m],
                                     xs0[:, s:s + w], start=True, stop=False)
                    nc.tensor.matmul(ps[ubase:ubase + m, :w],
                                     W["xp"][1][:, coff:coff + m],
                                     xs1[:, s:s + w], start=False, stop=True)
                nc.scalar.activation(u96rs[k][:, s:s + w],
                                     ps[rb:rb + RNK, :w], AF.Copy)
                bcsw = work.tile([112, MMC], bf16, tag="bcsw")
                nc.scalar.activation(bcsw[bb:bb + NST, :w], ps[bb:bb + NST, :w],
                                     AF.Copy)
                nc.scalar.activation(bcsw[cb:cb + NST, :w], ps[cb:cb + NST, :w],
                                     AF.Copy)
                nc.sync.dma_start(bcd[k, 0:NST, s:s + w], bcsw[bb:bb + NST, :w])
                nc.sync.dma_start(bcd[k, NST:32, s:s + w], bcsw[cb:cb + NST, :w])

        # ---- delta (Softplus) for both k, both dt ----
        dcs = []
        for k in range(2):
            W = kw[k]
            dc0 = big.tile([DT0, L], bf16, tag="xc0" if k == 0 else "xc1",
                           name=f"dc{k}0")
            dc1 = big.tile([DT1, L], bf16, tag=f"dc{k}1", name=f"dc{k}1")
            for (s, w) in MM:
                for dt, (rows, dct) in enumerate(((DT0, dc0), (DT1, dc1))):
                    ps = psM.tile([128, MMC], f32, tag="mm", name="psdt")
                    nc.tensor.matmul(ps[:rows, :w],
                                     W["dtw"][:, dt * DT0:dt * DT0 + rows],
                                     u96rs[k][:, s:s + w],
                                     start=True, stop=True)
                    nc.scalar.activation(dct[:, s:s + w], ps[:rows, :w],
                                         AF.Exp, bias=W["dtb"][dt][:])
            dcs.append((dc0, dc1))
        # batched in-place Ln phase: dc = ln(1 + e^u) = softplus(u)
        for k in range(2):
            for (s, w) in MM:
                for dt in range(2):
                    dct = dcs[k][dt]
                    nc.scalar.activation(dct[:, s:s + w], dct[:, s:s + w],
                                         AF.Ln, bias=1.0)

        # ---- P init with the direction-independent D term ----
        P0 = big.tile([DT0, L], bf16, tag="xm0", name="P0")
        P1 = big.tile([DT1, L], bf16, tag="xt1m", name="P1")
        nc.vector.tensor_scalar_mul(P0[:], xs0[:], dsum0[:])
        nc.gpsimd.tensor_scalar_mul(P1[:], xs1[:], dsum1[:])

        # ---- selective scan ----
        hp = [[scn.tile([128, NST, 1], bf16, tag=f"hp{k}{dt}", name=f"hp{k}{dt}")
               for dt in range(2)] for k in range(2)]
        for k in range(2):
            for dt in range(2):
                nc.vector.memset(hp[k][dt][:], 0.0)

        pairs = [(c, k) for c in range(len(SC)) for k in range(2)]
        bcr_tiles = {}

        def issue_bcast(i):
            c, k = pairs[i]
            s, w = SC[c]
            s0 = s if k == 0 else L - s - w
            t = scn.tile([128, 32, LC], bf16, tag="bcr", bufs=3,
                         name=f"bcr{i}")
            src = bass.AP(tensor=bcd, offset=k * 32 * L + s0,
                          ap=[[0, 128], [L, 32], [1, w]])
            nc.sync.dma_start(t[:], src)
            bcr_tiles[i] = t

        def tail(rows, dA, Ht, crep, hpt, pdst):
            # after the scan: save carry, G = H*C, PE n-reduction, P +=
            nc.gpsimd.tensor_copy(out=hpt[:rows], in_=Ht[:rows, :, LC:LC + 1])
            nc.vector.tensor_tensor(out=dA[:rows, :, 1:], in0=Ht[:rows, :, 1:],
                                    in1=crep[:rows], op=OP_.mult)
            psy = psY.tile([128, LC], f32, tag="psy", name="psy")
            for n in range(NST):
                nc.tensor.matmul(psy[:rows, :], eye[:rows, :rows],
                                 dA[:rows, n, 1:],
                                 start=(n == 0), stop=(n == NST - 1))
            nc.vector.tensor_tensor(out=pdst, in0=psy[:rows, :], in1=pdst,
                                    op=OP_.add)

        pending = []
        issue_bcast(0)
        for i, (c, k) in enumerate(pairs):
            s, w = SC[c]
            W = kw[k]
            bcr = bcr_tiles.pop(i)
            if k == 0:
                brep = bcr[:, 0:NST, :]
                crep = bcr[:, NST:32, :]
            else:
                brep = bcr[:, 0:NST, ::-1]
                crep = bcr[:, NST:32, ::-1]
            for dt, (rows, xst) in enumerate(((DT0, xs0), (DT1, xs1))):
                if len(pending) >= 3:
                    # flush the 3-iterations-old deferred tail before its
                    # dA/dBu/Ht buffer slots are reused below
                    pending.pop(0)()
                if dt == 0 and i + 1 < len(pairs):
                    # safe point: all readers of bcr slot (i+1)%3's previous
                    # occupant (pair i-2) have been emitted by now
                    issue_bcast(i + 1)
                dct = dcs[k][dt]
                dcsl = _sl(dct[:], k, s, w)
                xssl = _sl(xst[:], k, s, w)
                dxc = work.tile([128, LC], bf16, tag=f"dxc{dt}",
                                name=f"dxc{dt}", bufs=2)
                nc.gpsimd.tensor_tensor(out=dxc[:rows, :], in0=dcsl, in1=xssl,
                                        op=OP_.mult)
                dA = scn.tile([128, NST, LC + 1], bf16, tag="dA",
                              name=f"dA{dt}", bufs=3)
                dBu = scn.tile([128, NST, LC + 1], bf16, tag="dBu",
                               name=f"dBu{dt}", bufs=2)
                Ht = scn.tile([128, NST, LC + 1], bf16, tag="Ht",
                              name=f"Ht{dt}", bufs=3)
                nc.gpsimd.memset(dA[:rows, :, 0:1], 0.0)
                nc.gpsimd.tensor_copy(out=dBu[:rows, :, 0:1], in_=hp[k][dt][:rows])
                for n in range(NST):
                    nc.scalar.activation(dA[:rows, n, 1:], dcsl, AF.Exp,
                                         scale=W["ac"][dt][:, n:n + 1])
                beng = nc.vector if dt == 0 else nc.gpsimd
                beng.tensor_tensor(out=dBu[:rows, :, 1:],
                                   in0=_rep(dxc[:rows, :], NST),
                                   in1=brep[:rows], op=OP_.mult)
                nc.vector.tensor_tensor_scan(
                    out=Ht[:rows].rearrange("p a b -> p (a b)"),
                    data0=dA[:rows].rearrange("p a b -> p (a b)"),
                    data1=dBu[:rows].rearrange("p a b -> p (a b)"),
                    initial=0.0, op0=OP_.mult, op1=OP_.add)
                Pt = P0 if dt == 0 else P1
                args = (rows, dA, Ht, crep, hp[k][dt], _sl(Pt[:], k, s, w))
                pending.append(lambda a=args: tail(*a))
        for fn in pending:
            fn()

        # ---- Q = mrow*P + mcol*transpose(P) ----
        Q0 = big.tile([DT0, L], bf16, tag="xs0", name="Q0")
        Q1 = big.tile([DT1, L], bf16, tag="xs1", name="Q1")
        nc.vector.tensor_scalar_mul(Q0[:], _twh(P0[:]), mcol0[:])
        nc.vector.scalar_tensor_tensor(out=Q0[:], in0=P0[:], scalar=mrow0[:],
                                       in1=Q0[:], op0=OP_.mult, op1=OP_.add)
        nc.gpsimd.tensor_scalar_mul(Q1[:], _twh(P1[:]), mcol1[:])
        qtm = big.tile([DT1, L], bf16, tag="xm1", name="qtm")
        nc.gpsimd.tensor_scalar_mul(qtm[:], P1[:], mrow1[:])
        nc.gpsimd.tensor_tensor(out=Q1[:], in0=qtm[:], in1=Q1[:], op=OP_.add)
        nc.sync.dma_start(oq_d[0:DT0], Q0[:])
        nc.sync.dma_start(oq_d[DT0:DIN], Q1[:])
    nc.compile()
    return nc


# ---------------------------------------------------------------- pass 2
def build_nc2():
    nc = bacc.Bacc("TRN2", target_bir_lowering=False, debug=False, num_devices=8)
    din = {}

    def I(name, shape, dt=f32):
        din[name] = nc.dram_tensor(name, shape, dt, kind="ExternalInput")

    I("ym", [DIN, L], bf16); I("xin", [COUT, L]); I("zin", [DIN, L], bf16)
    I("OPm", [DIN, COUT], bf16); I("OPB", [DIN, COUT], bf16)
    I("PW1", [COUT, HID], bf16); I("g1", [HID, 1]); I("bb1", [HID, 1])
    I("cbdiag0", [DT0, 9, DT0], bf16); I("cbdiag1", [DT1, 9, DT1], bf16)
    I("g2", [HID, 1]); I("bb2", [HID, 1])
    I("PW2", [HID, COUT], bf16); I("g3", [COUT, 1]); I("bb3", [COUT, 1])
    I("fw", [COUT, 1]); I("fb", [COUT, 1])
    out_d = nc.dram_tensor("o", [COUT, L], f32, kind="ExternalOutput")

    ctx = contextlib.ExitStack()
    with tile.TileContext(nc) as tc, ctx:
        const = ctx.enter_context(tc.tile_pool(name="const", bufs=1))
        big = ctx.enter_context(tc.tile_pool(name="big", bufs=1))
        work = ctx.enter_context(tc.tile_pool(name="work", bufs=2))
        psM = ctx.enter_context(tc.tile_pool(name="psM", bufs=2, space="PSUM"))

        def load2(name, rows, cols, dt=f32):
            t0 = const.tile([DT0, cols], dt, tag=name + "0", name=name + "0")
            t1 = const.tile([DT1, cols], dt, tag=name + "1", name=name + "1")
            nc.sync.dma_start(t0[:], din[name][0:DT0])
            nc.sync.dma_start(t1[:], din[name][DT0:rows])
            return t0, t1

        def load1(name, rows):
            t = const.tile([rows, 1], f32, tag=name, name=name)
            nc.sync.dma_start(t[:], din[name][:])
            return t

        OP0, OP1 = load2("OPm", DIN, COUT, bf16)
        OPB0, OPB1 = load2("OPB", DIN, COUT, bf16)
        PW1t = const.tile([COUT, HID], bf16)
        nc.sync.dma_start(PW1t[:], din["PW1"][:])
        g1c0, g1c1 = load2("g1", HID, 1)
        bb1c0, bb1c1 = load2("bb1", HID, 1)
        cbd0 = const.tile([DT0, 9, DT0], bf16)
        nc.sync.dma_start(cbd0[:], din["cbdiag0"][:])
        cbd1 = const.tile([DT1, 9, DT1], bf16)
        nc.sync.dma_start(cbd1[:], din["cbdiag1"][:])
        g2c0, g2c1 = load2("g2", HID, 1)
        bb2c0, bb2c1 = load2("bb2", HID, 1)
        PW20, PW21 = load2("PW2", HID, COUT, bf16)
        g3c = load1("g3", COUT); bb3c = load1("bb3", COUT)
        fwc = load1("fw", COUT); fbc = load1("fb", COUT)
        onesb = const.tile([128, 1], bf16); nc.vector.memset(onesb[:], 1.0)
        onesrow = const.tile([1, 128], bf16); nc.vector.memset(onesrow[:], 1.0)
        epsc = const.tile([1, 1], f32); nc.vector.memset(epsc[:], EPS)

        ym0 = big.tile([DT0, L], bf16, tag="ym0")
        ym1 = big.tile([DT1, L], bf16, tag="ym1")
        nc.sync.dma_start(ym0[:], din["ym"][0:DT0])
        nc.sync.dma_start(ym1[:], din["ym"][DT0:DIN])
        xres = big.tile([COUT, L], f32, tag="xres")
        nc.sync.dma_start(xres[:], din["xin"][:])
        zc0 = big.tile([DT0, L], bf16, tag="zc0")
        zc1 = big.tile([DT1, L], bf16, tag="zc1")
        nc.sync.dma_start(zc0[:], din["zin"][0:DT0])
        nc.sync.dma_start(zc1[:], din["zin"][DT0:DIN])

        # out-norm stats over 192 partitions (batched, then one Rsqrt)
        mean_r = big.tile([1, L], bf16, tag="mean")
        ms_r = big.tile([1, L], bf16, tag="ms")
        for (s, w) in MM:
            ps = psM.tile([128, MMC], f32, tag="mm", name="pso1")
            nc.tensor.matmul(ps[:1, :w], onesb[:], ym0[:, s:s + w],
                             start=True, stop=False)
            nc.tensor.matmul(ps[:1, :w], onesb[:DT1], ym1[:, s:s + w],
                             start=False, stop=True)
            nc.scalar.activation(mean_r[:, s:s + w], ps[:1, :w], AF.Copy,
                                 scale=1.0 / DIN)
            ps2 = psM.tile([128, MMC], f32, tag="mm", name="pso2")
            for i, (t, rows) in enumerate(((ym0, DT0), (ym1, DT1))):
                sq = work.tile([128, MMC], bf16, tag="sqc", bufs=1)
                nc.vector.tensor_tensor(out=sq[:rows, :w], in0=t[:, s:s + w],
                                        in1=t[:, s:s + w], op=OP_.mult)
                nc.tensor.matmul(ps2[:1, :w], onesb[:rows], sq[:rows, :w],
                                 start=(i == 0), stop=(i == 1))
            nc.scalar.activation(ms_r[:, s:s + w], ps2[:1, :w], AF.Copy,
                                 scale=1.0 / DIN)
        msq = big.tile([1, L], f32, tag="msq")
        nc.vector.tensor_tensor(out=msq[:], in0=mean_r[:], in1=mean_r[:],
                                op=OP_.mult)
        nc.vector.tensor_tensor(out=msq[:], in0=ms_r[:], in1=msq[:],
                                op=OP_.subtract)
        nc.scalar.activation(msq[:], msq[:], AF.Sqrt, bias=epsc[:])
        rs_r = big.tile([1, L], bf16, tag="rs")
        with nc.allow_low_precision(reason="bf16 1/std is well conditioned"):
            nc.vector.reciprocal(rs_r[:], msq[:])

        x2f = big.tile([COUT, L], f32, tag="x2f")
        x2b = big.tile([COUT, L], bf16, tag="x2b")
        for (s, w) in MM:
            pm = psM.tile([128, MMC], f32, tag="mm", name="psm")
            nc.tensor.matmul(pm[:, :w], onesrow[:], mean_r[:, s:s + w],
                             start=True, stop=True)
            pr = psM.tile([128, MMC], f32, tag="mm", name="psr")
            nc.tensor.matmul(pr[:, :w], onesrow[:], rs_r[:, s:s + w],
                             start=True, stop=True)
            po = psM.tile([128, MMC], f32, tag="mm", name="pso")
            for i, (t, z, rows) in enumerate(((ym0, zc0, DT0), (ym1, zc1, DT1))):
                yn = work.tile([128, MMC], bf16, tag=f"yn{i}", name=f"yn{i}")
                nc.vector.tensor_tensor(out=yn[:rows, :w], in0=t[:, s:s + w],
                                        in1=pm[:rows, :w], op=OP_.subtract)
                nc.vector.tensor_tensor(out=yn[:rows, :w], in0=yn[:rows, :w],
                                        in1=pr[:rows, :w], op=OP_.mult)
                nc.vector.tensor_tensor(out=yn[:rows, :w], in0=yn[:rows, :w],
                                        in1=z[:, s:s + w], op=OP_.mult)
                OPt = OP0 if i == 0 else OP1
                OPBt = OPB0 if i == 0 else OPB1
                nc.tensor.matmul(po[:COUT, :w], OPt[:], yn[:rows, :w],
                                 start=(i == 0), stop=False)
                nc.tensor.matmul(po[:COUT, :w], OPBt[:], z[:, s:s + w],
                                 start=False, stop=(i == 1))
            nc.vector.tensor_tensor(out=x2f[:, s:s + w], in0=po[:COUT, :w],
                                    in1=xres[:, s:s + w], op=OP_.add)
            nc.vector.tensor_tensor(out=x2b[:, s:s + w], in0=po[:COUT, :w],
                                    in1=xres[:, s:s + w], op=OP_.add)

        # ConvBlock: PW1 + gelu
        t0 = big.tile([DT0, L], bf16, tag="ym0", name="t0")
        t1 = big.tile([DT1, L], bf16, tag="ym1", name="t1")
        for (s, w) in MM:
            for (dst, coff, rows, gc_, bc_) in ((t0, 0, DT0, g1c0, bb1c0),
                                                (t1, DT0, DT1, g1c1, bb1c1)):
                ps = psM.tile([128, MMC], f32, tag="mm", name="psp1")
                nc.tensor.matmul(ps[:rows, :w], PW1t[:, coff:coff + rows],
                                 x2b[:, s:s + w], start=True, stop=True)
                nc.scalar.activation(dst[:, s:s + w], ps[:rows, :w], AF.Gelu,
                                     bias=bc_[:], scale=gc_[:])
        # dw conv via PE; fused bn2+gelu on psum
        v0 = big.tile([DT0, L], bf16, tag="zc0", name="v0")
        v1 = big.tile([DT1, L], bf16, tag="zc1", name="v1")
        for (src, cd, rows, out, gc_, bc_) in (
                (t0, cbd0, DT0, v0, g2c0, bb2c0),
                (t1, cbd1, DT1, v1, g2c1, bb2c1)):
            pad = work.tile([128, 50, 50], bf16, tag="pad", bufs=1)
            nc.vector.memset(pad[:rows], 0.0)
            nc.vector.tensor_copy(out=pad[:rows, 1:49, 1:49], in_=_pl3(src[:]))
            for (y0, ny) in CROWS:
                pc = psM.tile([128, MMC], f32, tag="mm", name="pscv")
                for j in range(9):
                    dy, dx = divmod(j, 3)
                    view = pad[:rows, y0 + dy:y0 + dy + ny, dx:dx + 48]
                    nc.tensor.matmul(pc[:rows, :ny * 48], cd[:, j], view,
                                     start=(j == 0), stop=(j == 8))
                nc.scalar.activation(out[:, y0 * 48:(y0 + ny) * 48],
                                     pc[:rows, :ny * 48], AF.Gelu,
                                     bias=bc_[:], scale=gc_[:])
        # PW2 + bn3 + residual
        x3f = big.tile([COUT, L], f32, tag="x3f")
        x3b = big.tile([COUT, L], bf16, tag="xres", name="x3b")
        for (s, w) in MM:
            ps = psM.tile([128, MMC], f32, tag="mm", name="psp2")
            nc.tensor.matmul(ps[:COUT, :w], PW20[:], v0[:, s:s + w],
                             start=True, stop=False)
            nc.tensor.matmul(ps[:COUT, :w], PW21[:], v1[:, s:s + w],
                             start=False, stop=True)
            cbt = work.tile([128, MMC], bf16, tag="cbt", bufs=1)
            nc.scalar.activation(cbt[:COUT, :w], ps[:COUT, :w], AF.Identity,
                                 bias=bb3c[:], scale=g3c[:])
            nc.vector.tensor_tensor(out=x3f[:, s:s + w], in0=cbt[:COUT, :w],
                                    in1=x2f[:, s:s + w], op=OP_.add)
            nc.vector.tensor_tensor(out=x3b[:, s:s + w], in0=cbt[:COUT, :w],
                                    in1=x2f[:, s:s + w], op=OP_.add)

        # final LN
        mean2 = big.tile([1, L], bf16, tag="mean2")
        ms2 = big.tile([1, L], bf16, tag="ms2")
        for (s, w) in MM:
            ps = psM.tile([128, MMC], f32, tag="mm", name="psf1")
            nc.tensor.matmul(ps[:1, :w], onesb[:COUT], x3b[:, s:s + w],
                             start=True, stop=True)
            nc.scalar.activation(mean2[:, s:s + w], ps[:1, :w], AF.Copy,
                                 scale=1.0 / COUT)
            sq = work.tile([128, MMC], bf16, tag="sqc", bufs=1)
            nc.vector.tensor_tensor(out=sq[:COUT, :w], in0=x3b[:, s:s + w],
                                    in1=x3b[:, s:s + w], op=OP_.mult)
            ps2 = psM.tile([128, MMC], f32, tag="mm", name="psf2")
            nc.tensor.matmul(ps2[:1, :w], onesb[:COUT], sq[:COUT, :w],
                             start=True, stop=True)
            nc.scalar.activation(ms2[:, s:s + w], ps2[:1, :w], AF.Copy,
                                 scale=1.0 / COUT)
        msq2 = big.tile([1, L], f32, tag="msq2")
        nc.vector.tensor_tensor(out=msq2[:], in0=mean2[:], in1=mean2[:],
                                op=OP_.mult)
        nc.vector.tensor_tensor(out=msq2[:], in0=ms2[:], in1=msq2[:],
                                op=OP_.subtract)
        nc.scalar.activation(msq2[:], msq2[:], AF.Sqrt, bias=epsc[:])
        rs2 = big.tile([1, L], bf16, tag="rs2")
        with nc.allow_low_precision(reason="bf16 1/std is well conditioned"):
            nc.vector.reciprocal(rs2[:], msq2[:])
        for (s, w) in MM:
            pm = psM.tile([128, MMC], f32, tag="mm", name="psfm")
            nc.tensor.matmul(pm[:, :w], onesrow[:], mean2[:, s:s + w],
                             start=True, stop=True)
            pr = psM.tile([128, MMC], f32, tag="mm", name="psfr")
            nc.tensor.matmul(pr[:, :w], onesrow[:], rs2[:, s:s + w],
                             start=True, stop=True)
            oc = work.tile([128, MMC], f32, tag="oc", bufs=1)
            nc.vector.tensor_tensor(out=oc[:COUT, :w], in0=x3f[:, s:s + w],
                                    in1=pm[:COUT, :w], op=OP_.subtract)
            nc.vector.tensor_tensor(out=oc[:COUT, :w], in0=oc[:COUT, :w],
                                    in1=pr[:COUT, :w], op=OP_.mult)
            nc.vector.tensor_scalar(out=oc[:COUT, :w], in0=oc[:COUT, :w],
                                    scalar1=fwc[:], scalar2=fbc[:],
                                    op0=OP_.mult, op1=OP_.add)
            nc.sync.dma_start(out_d[:, s:s + w], oc[:COUT, :w])
    nc.compile()
    return nc


_NC1, _NC2 = None, None


def _get_ncs():
    global _NC1, _NC2
    if _NC1 is None:
        _NC1 = build_nc1()
        _NC2 = build_nc2()
    return _NC1, _NC2


def _bf(a):
    import jax.numpy as jnp
    return np.asarray(jnp.asarray(np.asarray(a, np.float32), jnp.bfloat16))


def _diag9(wmat, rows):
    out = np.zeros((rows, 9, rows), np.float32)
    idx = np.arange(rows)
    for j in range(9):
        out[idx, j, idx] = wmat[:, j]
    return out


def prep_pass1(ip):
    W1 = (np.diag(ip["ln1_w"]) @ ip["in_proj_W"]).astype(np.float32)
    b1 = (ip["ln1_b"] @ ip["in_proj_W"] + ip["in_proj_b"]).astype(np.float32)
    A = (-np.exp(ip["A_logs"].astype(np.float64))).astype(np.float32).reshape(KDIR, DIN, NST)
    Ds = ip["Ds"].reshape(KDIR, DIN)
    col = lambda v: np.ascontiguousarray(v.reshape(-1, 1), dtype=np.float32)
    convW = ip["conv_W"].reshape(DIN, 9)
    base = dict(projW=ip["proj_W"], projb=col(ip["proj_b"]), W1=_bf(W1),
                b1=col(b1),
                cdiag0=_bf(_diag9(convW[0:DT0], DT0)),
                cdiag1=_bf(_diag9(convW[DT0:DIN], DT1)),
                convb=col(ip["conv_b"]),
                eye=_bf(np.eye(128, dtype=np.float32)))
    maps = []
    for c in range(8):
        b, plane = c // 2, c % 2
        ks = [plane, plane + 2]
        m = dict(base)
        m["xc_t"] = np.ascontiguousarray(ip["x_cat"][b].reshape(L, CIN).T)
        m["xpw"] = _bf(np.stack([ip["x_proj_W"][k].T for k in ks]))
        m["dtw"] = _bf(np.stack([ip["dt_W"][k].T for k in ks]))
        m["dtb"] = np.ascontiguousarray(np.stack([col(ip["dt_b"][k]) for k in ks]))
        m["acoef"] = np.ascontiguousarray(np.stack([A[k] for k in ks]))
        m["dsum"] = col(Ds[ks[0]] + Ds[ks[1]])
        m["mrow"] = np.full((DIN, 1), 1.0 - plane, np.float32)
        m["mcol"] = np.full((DIN, 1), float(plane), np.float32)
        maps.append(m)
    return maps


def prep_pass2(ip, res1):
    OPm = (np.diag(ip["out_norm_w"]) @ ip["out_proj_W"]).astype(np.float32)
    OPB = (np.diag(ip["out_norm_b"]) @ ip["out_proj_W"]).astype(np.float32)
    col = lambda v: np.ascontiguousarray(v.reshape(-1, 1), dtype=np.float32)
    cbw = ip["cb_dw_W"].reshape(HID, 9)
    base = dict(OPm=_bf(OPm), OPB=_bf(OPB),
                PW1=_bf(ip["cb_pw1_W"][:, :, 0, 0].T),
                g1=col(ip["cb_bn1_g"]), bb1=col(ip["cb_bn1_b"]),
                cbdiag0=_bf(_diag9(cbw[0:DT0], DT0)),
                cbdiag1=_bf(_diag9(cbw[DT0:HID], DT1)),
                g2=col(ip["cb_bn2_g"]), bb2=col(ip["cb_bn2_b"]),
                PW2=_bf(ip["cb_pw2_W"][:, :, 0, 0].T),
                g3=col(ip["cb_bn3_g"]), bb3=col(ip["cb_bn3_b"]),
                fw=col(ip["norm_w"]), fb=col(ip["norm_b"]))
    maps = []
    for c in range(8):
        b = c // 2
        m = dict(base)
        ymf = (np.asarray(res1[2 * b]["oq"], np.float32)
               + np.asarray(res1[2 * b + 1]["oq"], np.float32))
        m["ym"] = _bf(ymf)
        m["xin"] = np.asarray(res1[2 * b]["ox"], np.float32)
        m["zin"] = np.ascontiguousarray(res1[2 * b]["oz"])
        maps.append(m)
    return maps


def kernel(**inputs):
    ip = {k: np.asarray(v, np.float32) for k, v in inputs.items()}
    nc1, nc2 = _get_ncs()
    res1 = run_bass_kernel_spmd(nc1, prep_pass1(ip), list(range(8))).results
    res2 = run_bass_kernel_spmd(nc2, prep_pass2(ip, res1), list(range(8))).results
    outs = [np.asarray(res2[2 * b]["o"], np.float32).T.reshape(H_, W_, COUT)
            for b in range(B_)]
    return np.stack(outs).astype(np.float32)


# revision 31
# speedup vs baseline: 1.7104x; 1.0173x over previous
"""Trainium2 Bass kernel for nn_DecoderFusionBlock (VSS/Mamba decoder fusion block).

Two-pass SPMD over 8 cores:
  pass 1: core c -> batch b=c//2, plane=c%2 (row-/col-major spatial order).
          proj/LN/in_proj (f32r / bf16 matmuls), depthwise conv via PE diag
          matmuls, then the selective scan for the plane's two directions.
          bf16 data path with fp32 scan state; B/C broadcast to all channel
          partitions via a DRAM-staged broadcast DMA so the big elementwise
          multiplies run in the DVE 2x (2-byte) mode; the n-state reduction
          runs on the PE as identity-weight matmul accumulation in PSUM.
  host:   ym[b] = Q[2b] + Q[2b+1]  (the only cross-core reduction)
  pass 2: core c -> batch b=c//2: out-norm, gate, out_proj+residual,
          ConvBlock (conv again via PE), final LayerNorm.
"""

import contextlib
import numpy as np

import concourse.bass as bass
import concourse.tile as tile
from concourse import bacc, mybir
from concourse.bass_utils import run_bass_kernel_spmd

f32 = mybir.dt.float32
f32r = mybir.dt.float32r
bf16 = mybir.dt.bfloat16
AF = mybir.ActivationFunctionType
OP_ = mybir.AluOpType

B_, H_, W_ = 4, 48, 48
L = H_ * W_
CIN, COUT = 192, 96
DIN, NST, RNK, KDIR = 192, 16, 6, 4
HID = 192
EPS = 1e-5
DT0, DT1 = 128, 64
MMC = 512
MM = [(s, min(MMC, L - s)) for s in range(0, L, MMC)]
LC = 256
SC = [(i * LC, LC) for i in range(L // LC)]
CROWS = [(0, 10), (10, 10), (20, 10), (30, 10), (40, 8)]


def _rev(ap, s, w):
    hi = L - 1 - s
    lo = hi - w
    return ap[:, hi::-1] if lo < 0 else ap[:, hi:lo:-1]


def _sl(ap, k, s, w):
    return ap[:, s:s + w] if k == 0 else _rev(ap, s, w)


def _rep(a, n):
    return bass.AP(tensor=a.tensor, offset=a.offset, ap=[a.ap[0], [0, n], a.ap[1]])


def _twh(a):
    st = a.ap[1][0]
    return bass.AP(tensor=a.tensor, offset=a.offset,
                   ap=[a.ap[0], [st, 48], [48 * st, 48]])


def _pl3(a):
    st = a.ap[1][0]
    return bass.AP(tensor=a.tensor, offset=a.offset,
                   ap=[a.ap[0], [48 * st, 48], [st, 48]])


# ---------------------------------------------------------------- pass 1
def build_nc1():
    nc = bacc.Bacc("TRN2", target_bir_lowering=False, debug=False, num_devices=8)
    din = {}

    def I(name, shape, dt=f32):
        din[name] = nc.dram_tensor(name, shape, dt, kind="ExternalInput")

    I("xc_t", [CIN, L], f32r)
    I("projW", [CIN, COUT], f32r); I("projb", [COUT, 1])
    I("W1", [COUT, 2 * DIN], bf16); I("b1", [2 * DIN, 1])
    I("cdiag0", [DT0, 9, DT0], bf16); I("cdiag1", [DT1, 9, DT1], bf16)
    I("convb", [DIN, 1])
    I("eye", [128, 128], bf16)
    I("xpw", [2, DIN, RNK + 2 * NST], bf16)
    I("dtw", [2, RNK, DIN], bf16)
    I("dtb", [2, DIN, 1]); I("acoef", [2, DIN, NST]); I("dsum", [DIN, 1])
    I("mrow", [DIN, 1]); I("mcol", [DIN, 1])
    oq_d = nc.dram_tensor("oq", [DIN, L], bf16, kind="ExternalOutput")
    ox_d = nc.dram_tensor("ox", [COUT, L], f32, kind="ExternalOutput")
    oz_d = nc.dram_tensor("oz", [DIN, L], bf16, kind="ExternalOutput")
    bcd = nc.dram_tensor("BCd", [2, 32, L], bf16, kind="Internal")

    ctx = contextlib.ExitStack()
    with tile.TileContext(nc) as tc, ctx:
        const = ctx.enter_context(tc.tile_pool(name="const", bufs=1))
        big = ctx.enter_context(tc.tile_pool(name="big", bufs=1))
        work = ctx.enter_context(tc.tile_pool(name="work", bufs=2))
        scn = ctx.enter_context(tc.tile_pool(name="scn", bufs=1))
        psM = ctx.enter_context(tc.tile_pool(name="psM", bufs=2, space="PSUM"))
        psY = ctx.enter_context(tc.tile_pool(name="psY", bufs=2, space="PSUM"))

        def load2(name, rows, cols, dt=f32):
            t0 = const.tile([DT0, cols], dt, tag=name + "0", name=name + "0")
            t1 = const.tile([DT1, cols], dt, tag=name + "1", name=name + "1")
            nc.sync.dma_start(t0[:], din[name][0:DT0])
            nc.sync.dma_start(t1[:], din[name][DT0:rows])
            return t0, t1

        projW0 = const.tile([DT0, COUT], f32r)
        projW1 = const.tile([DT1, COUT], f32r)
        nc.sync.dma_start(projW0[:], din["projW"][0:DT0])
        nc.sync.dma_start(projW1[:], din["projW"][DT0:CIN])
        projb = const.tile([COUT, 1], f32)
        nc.sync.dma_start(projb[:], din["projb"][:])
        W1t = const.tile([COUT, 2 * DIN], bf16)
        nc.sync.dma_start(W1t[:], din["W1"][:])
        b1x0 = const.tile([DT0, 1], f32); nc.sync.dma_start(b1x0[:], din["b1"][0:128])
        b1x1 = const.tile([DT1, 1], f32); nc.sync.dma_start(b1x1[:], din["b1"][128:192])
        b1z0 = const.tile([DT0, 1], f32); nc.sync.dma_start(b1z0[:], din["b1"][192:320])
        b1z1 = const.tile([DT1, 1], f32); nc.sync.dma_start(b1z1[:], din["b1"][320:384])
        cdiag0 = const.tile([DT0, 9, DT0], bf16)
        nc.sync.dma_start(cdiag0[:], din["cdiag0"][:])
        cdiag1 = const.tile([DT1, 9, DT1], bf16)
        nc.sync.dma_start(cdiag1[:], din["cdiag1"][:])
        convb0, convb1 = load2("convb", DIN, 1)
        eye = const.tile([128, 128], bf16)
        nc.sync.dma_start(eye[:], din["eye"][:])
        dsum0, dsum1 = load2("dsum", DIN, 1)
        mrow0, mrow1 = load2("mrow", DIN, 1)
        mcol0, mcol1 = load2("mcol", DIN, 1)
        kw = []
        for k in range(2):
            xp0 = const.tile([DT0, RNK + 2 * NST], bf16, name=f"xp{k}0")
            xp1 = const.tile([DT1, RNK + 2 * NST], bf16, name=f"xp{k}1")
            nc.sync.dma_start(xp0[:], din["xpw"][k, 0:DT0])
            nc.sync.dma_start(xp1[:], din["xpw"][k, DT0:DIN])
            dtw = const.tile([38, DIN], bf16, tag="dtwm", name=f"dtw{k}",
                             bufs=1) if k == 0 else kw[0]["dtwt"]
            nc.sync.dma_start(dtw[k * 32:k * 32 + RNK], din["dtw"][k])
            dtb0 = const.tile([DT0, 1], f32, name=f"dtb{k}0")
            dtb1 = const.tile([DT1, 1], f32, name=f"dtb{k}1")
            nc.sync.dma_start(dtb0[:], din["dtb"][k, 0:DT0])
            nc.sync.dma_start(dtb1[:], din["dtb"][k, DT0:DIN])
            ac0 = const.tile([DT0, NST], f32, name=f"ac{k}0")
            ac1 = const.tile([DT1, NST], f32, name=f"ac{k}1")
            nc.sync.dma_start(ac0[:], din["acoef"][k, 0:DT0])
            nc.sync.dma_start(ac1[:], din["acoef"][k, DT0:DIN])
            kw.append(dict(xp=(xp0, xp1), dtwt=dtw,
                           dtw=dtw[k * 32:k * 32 + RNK], dtb=(dtb0, dtb1),
                           ac=(ac0, ac1)))

        ones128 = const.tile([128, 1], f32); nc.vector.memset(ones128[:], 1.0)
        onesrow = const.tile([1, 128], bf16); nc.vector.memset(onesrow[:], 1.0)
        epsc = const.tile([1, 1], f32); nc.vector.memset(epsc[:], EPS)

        # ---- load + proj (f32r matmuls, x_t kept fp32 for residual) ----
        xc0 = big.tile([DT0, L], f32r, tag="xc0")
        xc1 = big.tile([DT1, L], f32r, tag="xc1")
        nc.sync.dma_start(xc0[:], din["xc_t"][0:DT0])
        nc.sync.dma_start(xc1[:], din["xc_t"][DT0:CIN])
        x_t = big.tile([COUT, L], f32, tag="x_t")
        for (s, w) in MM:
            ps = psM.tile([128, MMC], f32, tag="mm", name="psproj")
            nc.tensor.matmul(ps[:COUT, :w], projW0[:], xc0[:, s:s + w],
                             start=True, stop=False)
            nc.tensor.matmul(ps[:COUT, :w], projW1[:], xc1[:, s:s + w],
                             start=False, stop=True)
            nc.scalar.activation(x_t[:, s:s + w], ps[:COUT, :w], AF.Identity,
                                 bias=projb[:])
        nc.sync.dma_start(ox_d[:], x_t[:])

        # ---- LN1 (Copy + Sqrt share the act-table phase) -> xn bf16 ----
        xn_t = big.tile([COUT, L], bf16, tag="xn")
        for (s, w) in MM:
            ps1 = psM.tile([128, MMC], f32, tag="mm", name="pss1")
            nc.tensor.matmul(ps1[:1, :w], ones128[:COUT], x_t[:, s:s + w],
                             start=True, stop=True)
            mrw = work.tile([1, MMC], bf16, tag="mrw", bufs=1)
            nc.scalar.activation(mrw[:, :w], ps1[:1, :w], AF.Copy, scale=1.0 / COUT)
            sq = work.tile([128, MMC], f32, tag="sqc", bufs=1)
            nc.vector.tensor_tensor(out=sq[:COUT, :w], in0=x_t[:, s:s + w],
                                    in1=x_t[:, s:s + w], op=OP_.mult)
            ps2 = psM.tile([128, MMC], f32, tag="mm", name="pss2")
            nc.tensor.matmul(ps2[:1, :w], ones128[:COUT], sq[:COUT, :w],
                             start=True, stop=True)
            mq = work.tile([1, MMC], f32, tag="mq", bufs=1)
            nc.scalar.activation(mq[:, :w], ps2[:1, :w], AF.Copy, scale=1.0 / COUT)
            msq = work.tile([1, MMC], f32, tag="msq", bufs=1)
            nc.vector.tensor_tensor(out=msq[:, :w], in0=mrw[:, :w],
                                    in1=mrw[:, :w], op=OP_.mult)
            nc.vector.tensor_tensor(out=mq[:, :w], in0=mq[:, :w],
                                    in1=msq[:, :w], op=OP_.subtract)
            nc.scalar.activation(mq[:, :w], mq[:, :w], AF.Sqrt, bias=epsc[:])
            rsw = work.tile([1, MMC], bf16, tag="rsw", bufs=1)
            with nc.allow_low_precision(reason="bf16 1/std is well conditioned"):
                nc.vector.reciprocal(rsw[:, :w], mq[:, :w])
            pm = psM.tile([128, MMC], f32, tag="mm", name="psbm")
            nc.tensor.matmul(pm[:, :w], onesrow[:], mrw[:, :w],
                             start=True, stop=True)
            pr = psM.tile([128, MMC], f32, tag="mm", name="psbr")
            nc.tensor.matmul(pr[:, :w], onesrow[:], rsw[:, :w],
                             start=True, stop=True)
            xn_ = work.tile([128, MMC], bf16, tag="xn_", bufs=1)
            nc.vector.tensor_tensor(out=xn_[:COUT, :w], in0=x_t[:, s:s + w],
                                    in1=pm[:COUT, :w], op=OP_.subtract)
            nc.vector.tensor_tensor(out=xn_t[:, s:s + w], in0=xn_[:COUT, :w],
                                    in1=pr[:COUT, :w], op=OP_.mult)

        # ---- in_proj (xm tiles bf16; z silu'd -> DRAM bf16) ----
        xm0 = big.tile([DT0, L], bf16, tag="xm0")
        xm1 = big.tile([DT1, L], bf16, tag="xm1")
        for (s, w) in MM:
            for (coff, rows, bcol, dst, zoff) in (
                    (0, DT0, b1x0, xm0, None), (DT0, DT1, b1x1, xm1, None),
                    (DIN, DT0, b1z0, None, 0), (DIN + DT0, DT1, b1z1, None, DT0)):
                psi = psM.tile([128, MMC], f32, tag="mm", name="psip")
                nc.tensor.matmul(psi[:rows, :w], W1t[:, coff:coff + rows],
                                 xn_t[:, s:s + w], start=True, stop=True)
                if dst is not None:
                    nc.scalar.activation(dst[:, s:s + w], psi[:rows, :w],
                                         AF.Identity, bias=bcol[:])
                else:
                    zc = work.tile([128, MMC], bf16, tag="zc", bufs=1)
                    nc.scalar.activation(zc[:rows, :w], psi[:rows, :w], AF.Silu,
                                         bias=bcol[:])
                    nc.sync.dma_start(oz_d[zoff:zoff + rows, s:s + w], zc[:rows, :w])

        # ---- depthwise conv via PE diag matmuls + fused SiLU; the
        #      transposed copy for the plane transform happens per row-chunk
        xs0 = big.tile([DT0, L], bf16, tag="xs0")
        xs1 = big.tile([DT1, L], bf16, tag="xs1")
        xt0 = big.tile([DT0, L], bf16, tag="xm0", name="xt0")
        xt1 = big.tile([DT1, L], bf16, tag="xm1", name="xt1")
        for (src, cd, rows, out, bcol, xtt, teng) in (
                (xm0, cdiag0, DT0, xs0, convb0, xt0, nc.vector),
                (xm1, cdiag1, DT1, xs1, convb1, xt1, nc.vector)):
            pad = work.tile([128, 50, 50], bf16, tag="pad", bufs=1)
            nc.vector.memset(pad[:rows], 0.0)
            nc.vector.tensor_copy(out=pad[:rows, 1:49, 1:49], in_=_pl3(src[:]))
            for (y0, ny) in CROWS:
                pc = psM.tile([128, MMC], f32, tag="mm", name="pscv")
                for j in range(9):
                    dy, dx = divmod(j, 3)
                    view = pad[:rows, y0 + dy:y0 + dy + ny, dx:dx + 48]
                    nc.tensor.matmul(pc[:rows, :ny * 48], cd[:, j], view,
                                     start=(j == 0), stop=(j == 8))
                nc.scalar.activation(out[:, y0 * 48:(y0 + ny) * 48],
                                     pc[:rows, :ny * 48], AF.Silu, bias=bcol[:])
                # xt[p, x, y] = xs[p, y, x] for this y-chunk
                xin = _pl3(out[:])[:, y0:y0 + ny, :]
                xout = bass.AP(tensor=xtt.tensor, offset=xtt[:].offset + y0,
                               ap=[xtt[:].ap[0], [1, ny], [48, 48]])
                teng.tensor_copy(out=xout, in_=xin)
        nc.vector.tensor_scalar_mul(xs0[:], xs0[:], mrow0[:])
        nc.vector.scalar_tensor_tensor(out=xs0[:], in0=xt0[:], scalar=mcol0[:],
                                       in1=xs0[:], op0=OP_.mult, op1=OP_.add)
        nc.vector.tensor_scalar_mul(xs1[:], xs1[:], mrow1[:])
        nc.vector.scalar_tensor_tensor(out=xs1[:], in0=xt1[:], scalar=mcol1[:],
                                       in1=xs1[:], op0=OP_.mult, op1=OP_.add)

        # ---- U96: x_dbl for both k; B/C staged to DRAM bf16 ----
        u96m = big.tile([38, L], bf16, tag="u96r")
        u96rs = (u96m[0:RNK], u96m[32:38])
        for k in range(2):
            W = kw[k]
            rb = k * 32          # rank section base: 0 (k0) / 32 (k1)
            bb = rb + 32
            cb = 64 if k == 0 else 0
            for (s, w) in MM:
                ps = psM.tile([128, MMC], f32, tag="mm", name="psU")
                for (coff, ubase, m) in ((0, rb, RNK), (RNK, bb, NST),
                                         (RNK + NST, cb, NST)):
                    nc.tensor.matmul(ps[ubase:ubase + m, :w],
                                     W["xp"][0][:, coff:coff + m],
                                     xs0[:, s:s + w], start=True, stop=False)
                    nc.tensor.matmul(ps[ubase:ubase + m, :w],
                                     W["xp"][1][:, coff:coff + m],
                                     xs1[:, s:s + w], start=False, stop=True)
                nc.scalar.activation(u96rs[k][:, s:s + w],
                                     ps[rb:rb + RNK, :w], AF.Copy)
                bcsw = work.tile([112, MMC], bf16, tag="bcsw")
                nc.scalar.activation(bcsw[bb:bb + NST, :w], ps[bb:bb + NST, :w],
                                     AF.Copy)
                nc.scalar.activation(bcsw[cb:cb + NST, :w], ps[cb:cb + NST, :w],
                                     AF.Copy)
                nc.sync.dma_start(bcd[k, 0:NST, s:s + w], bcsw[bb:bb + NST, :w])
                nc.sync.dma_start(bcd[k, NST:32, s:s + w], bcsw[cb:cb + NST, :w])

        # ---- delta (Softplus) for both k, both dt ----
        dcs = []
        for k in range(2):
            W = kw[k]
            dc0 = big.tile([DT0, L], bf16, tag="xc0" if k == 0 else "xc1",
                           name=f"dc{k}0")
            dc1 = big.tile([DT1, L], bf16, tag=f"dc{k}1", name=f"dc{k}1")
            for (s, w) in MM:
                for dt, (rows, dct) in enumerate(((DT0, dc0), (DT1, dc1))):
                    ps = psM.tile([128, MMC], f32, tag="mm", name="psdt")
                    nc.tensor.matmul(ps[:rows, :w],
                                     W["dtw"][:, dt * DT0:dt * DT0 + rows],
                                     u96rs[k][:, s:s + w],
                                     start=True, stop=True)
                    nc.scalar.activation(dct[:, s:s + w], ps[:rows, :w],
                                         AF.Exp, bias=W["dtb"][dt][:])
            # in-place Ln: dc = ln(1 + e^u) = softplus(u), batched per k
            for (s, w) in MM:
                for dct in (dc0, dc1):
                    nc.scalar.activation(dct[:, s:s + w], dct[:, s:s + w],
                                         AF.Ln, bias=1.0)
            dcs.append((dc0, dc1))

        # ---- P init with the direction-independent D term ----
        P0 = big.tile([DT0, L], bf16, tag="xm0", name="P0")
        P1 = big.tile([DT1, L], bf16, tag="xt1m", name="P1")
        nc.vector.tensor_scalar_mul(P0[:], xs0[:], dsum0[:])
        nc.gpsimd.tensor_scalar_mul(P1[:], xs1[:], dsum1[:])

        # ---- selective scan ----
        hp = [[scn.tile([128, NST, 1], bf16, tag=f"hp{k}{dt}", name=f"hp{k}{dt}")
               for dt in range(2)] for k in range(2)]
        for k in range(2):
            for dt in range(2):
                nc.vector.memset(hp[k][dt][:], 0.0)

        pairs = [(c, k) for c in range(len(SC)) for k in range(2)]
        bcr_tiles = {}

        def issue_bcast(i):
            c, k = pairs[i]
            s, w = SC[c]
            s0 = s if k == 0 else L - s - w
            t = scn.tile([128, 32, LC], bf16, tag="bcr", bufs=3,
                         name=f"bcr{i}")
            src = bass.AP(tensor=bcd, offset=k * 32 * L + s0,
                          ap=[[0, 128], [L, 32], [1, w]])
            nc.sync.dma_start(t[:], src)
            bcr_tiles[i] = t

        def tail(rows, dA, Ht, crep, hpt, pdst):
            # after the scan: save carry, G = H*C, PE n-reduction, P +=
            nc.gpsimd.tensor_copy(out=hpt[:rows], in_=Ht[:rows, :, LC:LC + 1])
            nc.vector.tensor_tensor(out=dA[:rows, :, 1:], in0=Ht[:rows, :, 1:],
                                    in1=crep[:rows], op=OP_.mult)
            psy = psY.tile([128, LC], f32, tag="psy", name="psy")
            for n in range(NST):
                nc.tensor.matmul(psy[:rows, :], eye[:rows, :rows],
                                 dA[:rows, n, 1:],
                                 start=(n == 0), stop=(n == NST - 1))
            nc.vector.tensor_tensor(out=pdst, in0=psy[:rows, :], in1=pdst,
                                    op=OP_.add)

        pending = []
        issue_bcast(0)
        for i, (c, k) in enumerate(pairs):
            s, w = SC[c]
            W = kw[k]
            bcr = bcr_tiles.pop(i)
            if k == 0:
                brep = bcr[:, 0:NST, :]
                crep = bcr[:, NST:32, :]
            else:
                brep = bcr[:, 0:NST, ::-1]
                crep = bcr[:, NST:32, ::-1]
            for dt, (rows, xst) in enumerate(((DT0, xs0), (DT1, xs1))):
                if len(pending) >= 3:
                    # flush the 3-iterations-old deferred tail before its
                    # dA/dBu/Ht buffer slots are reused below
                    pending.pop(0)()
                if dt == 0 and i + 1 < len(pairs):
                    # safe point: all readers of bcr slot (i+1)%3's previous
                    # occupant (pair i-2) have been emitted by now
                    issue_bcast(i + 1)
                dct = dcs[k][dt]
                dcsl = _sl(dct[:], k, s, w)
                xssl = _sl(xst[:], k, s, w)
                dxc = work.tile([128, LC], bf16, tag=f"dxc{dt}",
                                name=f"dxc{dt}", bufs=2)
                nc.gpsimd.tensor_tensor(out=dxc[:rows, :], in0=dcsl, in1=xssl,
                                        op=OP_.mult)
                dA = scn.tile([128, NST, LC + 1], bf16, tag="dA",
                              name=f"dA{dt}", bufs=3)
                dBu = scn.tile([128, NST, LC + 1], bf16, tag="dBu",
                               name=f"dBu{dt}", bufs=2)
                Ht = scn.tile([128, NST, LC + 1], bf16, tag="Ht",
                              name=f"Ht{dt}", bufs=3)
                nc.gpsimd.memset(dA[:rows, :, 0:1], 0.0)
                nc.gpsimd.tensor_copy(out=dBu[:rows, :, 0:1], in_=hp[k][dt][:rows])
                for n in range(NST):
                    nc.scalar.activation(dA[:rows, n, 1:], dcsl, AF.Exp,
                                         scale=W["ac"][dt][:, n:n + 1])
                beng = nc.vector if dt == 0 else nc.gpsimd
                beng.tensor_tensor(out=dBu[:rows, :, 1:],
                                   in0=_rep(dxc[:rows, :], NST),
                                   in1=brep[:rows], op=OP_.mult)
                nc.vector.tensor_tensor_scan(
                    out=Ht[:rows].rearrange("p a b -> p (a b)"),
                    data0=dA[:rows].rearrange("p a b -> p (a b)"),
                    data1=dBu[:rows].rearrange("p a b -> p (a b)"),
                    initial=0.0, op0=OP_.mult, op1=OP_.add)
                Pt = P0 if dt == 0 else P1
                args = (rows, dA, Ht, crep, hp[k][dt], _sl(Pt[:], k, s, w))
                pending.append(lambda a=args: tail(*a))
        for fn in pending:
            fn()

        # ---- Q = mrow*P + mcol*transpose(P) ----
        Q0 = big.tile([DT0, L], bf16, tag="xs0", name="Q0")
        Q1 = big.tile([DT1, L], bf16, tag="xs1", name="Q1")
        nc.vector.tensor_scalar_mul(Q0[:], _twh(P0[:]), mcol0[:])
        nc.vector.scalar_tensor_tensor(out=Q0[:], in0=P0[:], scalar=mrow0[:],
                                       in1=Q0[:], op0=OP_.mult, op1=OP_.add)
        nc.gpsimd.tensor_scalar_mul(Q1[:], _twh(P1[:]), mcol1[:])
        qtm = big.tile([DT1, L], bf16, tag="xm1", name="qtm")
        nc.gpsimd.tensor_scalar_mul(qtm[:], P1[:], mrow1[:])
        nc.gpsimd.tensor_tensor(out=Q1[:], in0=qtm[:], in1=Q1[:], op=OP_.add)
        nc.sync.dma_start(oq_d[0:DT0], Q0[:])
        nc.sync.dma_start(oq_d[DT0:DIN], Q1[:])
    nc.compile()
    return nc


# ---------------------------------------------------------------- pass 2
def build_nc2():
    nc = bacc.Bacc("TRN2", target_bir_lowering=False, debug=False, num_devices=8)
    din = {}

    def I(name, shape, dt=f32):
        din[name] = nc.dram_tensor(name, shape, dt, kind="ExternalInput")

    I("ym", [DIN, L], bf16); I("xin", [COUT, L]); I("zin", [DIN, L], bf16)
    I("OPm", [DIN, COUT], bf16); I("OPB", [DIN, COUT], bf16)
    I("PW1", [COUT, HID], bf16); I("g1", [HID, 1]); I("bb1", [HID, 1])
    I("cbdiag0", [DT0, 9, DT0], bf16); I("cbdiag1", [DT1, 9, DT1], bf16)
    I("g2", [HID, 1]); I("bb2", [HID, 1])
    I("PW2", [HID, COUT], bf16); I("g3", [COUT, 1]); I("bb3", [COUT, 1])
    I("fw", [COUT, 1]); I("fb", [COUT, 1])
    out_d = nc.dram_tensor("o", [COUT, L], f32, kind="ExternalOutput")

    ctx = contextlib.ExitStack()
    with tile.TileContext(nc) as tc, ctx:
        const = ctx.enter_context(tc.tile_pool(name="const", bufs=1))
        big = ctx.enter_context(tc.tile_pool(name="big", bufs=1))
        work = ctx.enter_context(tc.tile_pool(name="work", bufs=2))
        psM = ctx.enter_context(tc.tile_pool(name="psM", bufs=2, space="PSUM"))

        def load2(name, rows, cols, dt=f32):
            t0 = const.tile([DT0, cols], dt, tag=name + "0", name=name + "0")
            t1 = const.tile([DT1, cols], dt, tag=name + "1", name=name + "1")
            nc.sync.dma_start(t0[:], din[name][0:DT0])
            nc.sync.dma_start(t1[:], din[name][DT0:rows])
            return t0, t1

        def load1(name, rows):
            t = const.tile([rows, 1], f32, tag=name, name=name)
            nc.sync.dma_start(t[:], din[name][:])
            return t

        OP0, OP1 = load2("OPm", DIN, COUT, bf16)
        OPB0, OPB1 = load2("OPB", DIN, COUT, bf16)
        PW1t = const.tile([COUT, HID], bf16)
        nc.sync.dma_start(PW1t[:], din["PW1"][:])
        g1c0, g1c1 = load2("g1", HID, 1)
        bb1c0, bb1c1 = load2("bb1", HID, 1)
        cbd0 = const.tile([DT0, 9, DT0], bf16)
        nc.sync.dma_start(cbd0[:], din["cbdiag0"][:])
        cbd1 = const.tile([DT1, 9, DT1], bf16)
        nc.sync.dma_start(cbd1[:], din["cbdiag1"][:])
        g2c0, g2c1 = load2("g2", HID, 1)
        bb2c0, bb2c1 = load2("bb2", HID, 1)
        PW20, PW21 = load2("PW2", HID, COUT, bf16)
        g3c = load1("g3", COUT); bb3c = load1("bb3", COUT)
        fwc = load1("fw", COUT); fbc = load1("fb", COUT)
        onesb = const.tile([128, 1], bf16); nc.vector.memset(onesb[:], 1.0)
        onesrow = const.tile([1, 128], bf16); nc.vector.memset(onesrow[:], 1.0)
        epsc = const.tile([1, 1], f32); nc.vector.memset(epsc[:], EPS)

        ym0 = big.tile([DT0, L], bf16, tag="ym0")
        ym1 = big.tile([DT1, L], bf16, tag="ym1")
        nc.sync.dma_start(ym0[:], din["ym"][0:DT0])
        nc.sync.dma_start(ym1[:], din["ym"][DT0:DIN])
        xres = big.tile([COUT, L], f32, tag="xres")
        nc.sync.dma_start(xres[:], din["xin"][:])
        zc0 = big.tile([DT0, L], bf16, tag="zc0")
        zc1 = big.tile([DT1, L], bf16, tag="zc1")
        nc.sync.dma_start(zc0[:], din["zin"][0:DT0])
        nc.sync.dma_start(zc1[:], din["zin"][DT0:DIN])

        # out-norm stats over 192 partitions (batched, then one Rsqrt)
        mean_r = big.tile([1, L], bf16, tag="mean")
        rs_r = big.tile([1, L], bf16, tag="rs")
        for (s, w) in MM:
            ps = psM.tile([128, MMC], f32, tag="mm", name="pso1")
            nc.tensor.matmul(ps[:1, :w], onesb[:], ym0[:, s:s + w],
                             start=True, stop=False)
            nc.tensor.matmul(ps[:1, :w], onesb[:DT1], ym1[:, s:s + w],
                             start=False, stop=True)
            nc.scalar.activation(mean_r[:, s:s + w], ps[:1, :w], AF.Copy,
                                 scale=1.0 / DIN)
            ps2 = psM.tile([128, MMC], f32, tag="mm", name="pso2")
            for i, (t, rows) in enumerate(((ym0, DT0), (ym1, DT1))):
                sq = work.tile([128, MMC], bf16, tag="sqc", bufs=1)
                nc.vector.tensor_tensor(out=sq[:rows, :w], in0=t[:, s:s + w],
                                        in1=t[:, s:s + w], op=OP_.mult)
                nc.tensor.matmul(ps2[:1, :w], onesb[:rows], sq[:rows, :w],
                                 start=(i == 0), stop=(i == 1))
            mq = work.tile([1, MMC], f32, tag="mq", bufs=1)
            nc.scalar.activation(mq[:, :w], ps2[:1, :w], AF.Copy,
                                 scale=1.0 / DIN)
            msqc = work.tile([1, MMC], f32, tag="msqc", bufs=1)
            nc.vector.tensor_tensor(out=msqc[:, :w], in0=mean_r[:, s:s + w],
                                    in1=mean_r[:, s:s + w], op=OP_.mult)
            nc.vector.tensor_tensor(out=mq[:, :w], in0=mq[:, :w],
                                    in1=msqc[:, :w], op=OP_.subtract)
            nc.scalar.activation(mq[:, :w], mq[:, :w], AF.Sqrt, bias=epsc[:])
            with nc.allow_low_precision(reason="bf16 1/std is well conditioned"):
                nc.vector.reciprocal(rs_r[:, s:s + w], mq[:, :w])

        x2f = big.tile([COUT, L], f32, tag="x2f")
        x2b = big.tile([COUT, L], bf16, tag="x2b")
        for (s, w) in MM:
            pm = psM.tile([128, MMC], f32, tag="mm", name="psm")
            nc.tensor.matmul(pm[:, :w], onesrow[:], mean_r[:, s:s + w],
                             start=True, stop=True)
            pr = psM.tile([128, MMC], f32, tag="mm", name="psr")
            nc.tensor.matmul(pr[:, :w], onesrow[:], rs_r[:, s:s + w],
                             start=True, stop=True)
            po = psM.tile([128, MMC], f32, tag="mm", name="pso")
            for i, (t, z, rows) in enumerate(((ym0, zc0, DT0), (ym1, zc1, DT1))):
                yn = work.tile([128, MMC], bf16, tag=f"yn{i}", name=f"yn{i}")
                nc.vector.tensor_tensor(out=yn[:rows, :w], in0=t[:, s:s + w],
                                        in1=pm[:rows, :w], op=OP_.subtract)
                nc.vector.tensor_tensor(out=yn[:rows, :w], in0=yn[:rows, :w],
                                        in1=pr[:rows, :w], op=OP_.mult)
                nc.vector.tensor_tensor(out=yn[:rows, :w], in0=yn[:rows, :w],
                                        in1=z[:, s:s + w], op=OP_.mult)
                OPt = OP0 if i == 0 else OP1
                OPBt = OPB0 if i == 0 else OPB1
                nc.tensor.matmul(po[:COUT, :w], OPt[:], yn[:rows, :w],
                                 start=(i == 0), stop=False)
                nc.tensor.matmul(po[:COUT, :w], OPBt[:], z[:, s:s + w],
                                 start=False, stop=(i == 1))
            nc.vector.tensor_tensor(out=x2f[:, s:s + w], in0=po[:COUT, :w],
                                    in1=xres[:, s:s + w], op=OP_.add)
            nc.vector.tensor_tensor(out=x2b[:, s:s + w], in0=po[:COUT, :w],
                                    in1=xres[:, s:s + w], op=OP_.add)

        # ConvBlock: PW1 + gelu
        t0 = big.tile([DT0, L], bf16, tag="ym0", name="t0")
        t1 = big.tile([DT1, L], bf16, tag="ym1", name="t1")
        for (s, w) in MM:
            for (dst, coff, rows, gc_, bc_) in ((t0, 0, DT0, g1c0, bb1c0),
                                                (t1, DT0, DT1, g1c1, bb1c1)):
                ps = psM.tile([128, MMC], f32, tag="mm", name="psp1")
                nc.tensor.matmul(ps[:rows, :w], PW1t[:, coff:coff + rows],
                                 x2b[:, s:s + w], start=True, stop=True)
                nc.scalar.activation(dst[:, s:s + w], ps[:rows, :w], AF.Gelu,
                                     bias=bc_[:], scale=gc_[:])
        # dw conv via PE; fused bn2+gelu on psum
        v0 = big.tile([DT0, L], bf16, tag="zc0", name="v0")
        v1 = big.tile([DT1, L], bf16, tag="zc1", name="v1")
        for (src, cd, rows, out, gc_, bc_) in (
                (t0, cbd0, DT0, v0, g2c0, bb2c0),
                (t1, cbd1, DT1, v1, g2c1, bb2c1)):
            pad = work.tile([128, 50, 50], bf16, tag="pad", bufs=1)
            nc.vector.memset(pad[:rows], 0.0)
            nc.vector.tensor_copy(out=pad[:rows, 1:49, 1:49], in_=_pl3(src[:]))
            for (y0, ny) in CROWS:
                pc = psM.tile([128, MMC], f32, tag="mm", name="pscv")
                for j in range(9):
                    dy, dx = divmod(j, 3)
                    view = pad[:rows, y0 + dy:y0 + dy + ny, dx:dx + 48]
                    nc.tensor.matmul(pc[:rows, :ny * 48], cd[:, j], view,
                                     start=(j == 0), stop=(j == 8))
                nc.scalar.activation(out[:, y0 * 48:(y0 + ny) * 48],
                                     pc[:rows, :ny * 48], AF.Gelu,
                                     bias=bc_[:], scale=gc_[:])
        # PW2 + bn3 + residual
        x3f = big.tile([COUT, L], f32, tag="x3f")
        x3b = big.tile([COUT, L], bf16, tag="xres", name="x3b")
        for (s, w) in MM:
            ps = psM.tile([128, MMC], f32, tag="mm", name="psp2")
            nc.tensor.matmul(ps[:COUT, :w], PW20[:], v0[:, s:s + w],
                             start=True, stop=False)
            nc.tensor.matmul(ps[:COUT, :w], PW21[:], v1[:, s:s + w],
                             start=False, stop=True)
            cbt = work.tile([128, MMC], bf16, tag="cbt", bufs=1)
            nc.scalar.activation(cbt[:COUT, :w], ps[:COUT, :w], AF.Identity,
                                 bias=bb3c[:], scale=g3c[:])
            nc.vector.tensor_tensor(out=x3f[:, s:s + w], in0=cbt[:COUT, :w],
                                    in1=x2f[:, s:s + w], op=OP_.add)
            nc.vector.tensor_tensor(out=x3b[:, s:s + w], in0=cbt[:COUT, :w],
                                    in1=x2f[:, s:s + w], op=OP_.add)

        # final LN
        mean2 = big.tile([1, L], bf16, tag="mean2")
        rs2 = big.tile([1, L], bf16, tag="rs2")
        for (s, w) in MM:
            ps = psM.tile([128, MMC], f32, tag="mm", name="psf1")
            nc.tensor.matmul(ps[:1, :w], onesb[:COUT], x3b[:, s:s + w],
                             start=True, stop=True)
            nc.scalar.activation(mean2[:, s:s + w], ps[:1, :w], AF.Copy,
                                 scale=1.0 / COUT)
            sq = work.tile([128, MMC], bf16, tag="sqc", bufs=1)
            nc.vector.tensor_tensor(out=sq[:COUT, :w], in0=x3b[:, s:s + w],
                                    in1=x3b[:, s:s + w], op=OP_.mult)
            ps2 = psM.tile([128, MMC], f32, tag="mm", name="psf2")
            nc.tensor.matmul(ps2[:1, :w], onesb[:COUT], sq[:COUT, :w],
                             start=True, stop=True)
            mq2 = work.tile([1, MMC], f32, tag="mq2", bufs=1)
            nc.scalar.activation(mq2[:, :w], ps2[:1, :w], AF.Copy,
                                 scale=1.0 / COUT)
            msqc2 = work.tile([1, MMC], f32, tag="msqc2", bufs=1)
            nc.vector.tensor_tensor(out=msqc2[:, :w], in0=mean2[:, s:s + w],
                                    in1=mean2[:, s:s + w], op=OP_.mult)
            nc.vector.tensor_tensor(out=mq2[:, :w], in0=mq2[:, :w],
                                    in1=msqc2[:, :w], op=OP_.subtract)
            nc.scalar.activation(mq2[:, :w], mq2[:, :w], AF.Sqrt, bias=epsc[:])
            with nc.allow_low_precision(reason="bf16 1/std is well conditioned"):
                nc.vector.reciprocal(rs2[:, s:s + w], mq2[:, :w])
        for (s, w) in MM:
            pm = psM.tile([128, MMC], f32, tag="mm", name="psfm")
            nc.tensor.matmul(pm[:, :w], onesrow[:], mean2[:, s:s + w],
                             start=True, stop=True)
            pr = psM.tile([128, MMC], f32, tag="mm", name="psfr")
            nc.tensor.matmul(pr[:, :w], onesrow[:], rs2[:, s:s + w],
                             start=True, stop=True)
            oc = work.tile([128, MMC], f32, tag="oc", bufs=1)
            nc.vector.tensor_tensor(out=oc[:COUT, :w], in0=x3f[:, s:s + w],
                                    in1=pm[:COUT, :w], op=OP_.subtract)
            nc.vector.tensor_tensor(out=oc[:COUT, :w], in0=oc[:COUT, :w],
                                    in1=pr[:COUT, :w], op=OP_.mult)
            nc.vector.tensor_scalar(out=oc[:COUT, :w], in0=oc[:COUT, :w],
                                    scalar1=fwc[:], scalar2=fbc[:],
                                    op0=OP_.mult, op1=OP_.add)
            nc.sync.dma_start(out_d[:, s:s + w], oc[:COUT, :w])
    nc.compile()
    return nc


_NC1, _NC2 = None, None


def _get_ncs():
    global _NC1, _NC2
    if _NC1 is None:
        _NC1 = build_nc1()
        _NC2 = build_nc2()
    return _NC1, _NC2


def _bf(a):
    import jax.numpy as jnp
    return np.asarray(jnp.asarray(np.asarray(a, np.float32), jnp.bfloat16))


def _diag9(wmat, rows):
    out = np.zeros((rows, 9, rows), np.float32)
    idx = np.arange(rows)
    for j in range(9):
        out[idx, j, idx] = wmat[:, j]
    return out


def prep_pass1(ip):
    W1 = (np.diag(ip["ln1_w"]) @ ip["in_proj_W"]).astype(np.float32)
    b1 = (ip["ln1_b"] @ ip["in_proj_W"] + ip["in_proj_b"]).astype(np.float32)
    A = (-np.exp(ip["A_logs"].astype(np.float64))).astype(np.float32).reshape(KDIR, DIN, NST)
    Ds = ip["Ds"].reshape(KDIR, DIN)
    col = lambda v: np.ascontiguousarray(v.reshape(-1, 1), dtype=np.float32)
    convW = ip["conv_W"].reshape(DIN, 9)
    base = dict(projW=ip["proj_W"], projb=col(ip["proj_b"]), W1=_bf(W1),
                b1=col(b1),
                cdiag0=_bf(_diag9(convW[0:DT0], DT0)),
                cdiag1=_bf(_diag9(convW[DT0:DIN], DT1)),
                convb=col(ip["conv_b"]),
                eye=_bf(np.eye(128, dtype=np.float32)))
    maps = []
    for c in range(8):
        b, plane = c // 2, c % 2
        ks = [plane, plane + 2]
        m = dict(base)
        m["xc_t"] = np.ascontiguousarray(ip["x_cat"][b].reshape(L, CIN).T)
        m["xpw"] = _bf(np.stack([ip["x_proj_W"][k].T for k in ks]))
        m["dtw"] = _bf(np.stack([ip["dt_W"][k].T for k in ks]))
        m["dtb"] = np.ascontiguousarray(np.stack([col(ip["dt_b"][k]) for k in ks]))
        m["acoef"] = np.ascontiguousarray(np.stack([A[k] for k in ks]))
        m["dsum"] = col(Ds[ks[0]] + Ds[ks[1]])
        m["mrow"] = np.full((DIN, 1), 1.0 - plane, np.float32)
        m["mcol"] = np.full((DIN, 1), float(plane), np.float32)
        maps.append(m)
    return maps


def prep_pass2(ip, res1):
    OPm = (np.diag(ip["out_norm_w"]) @ ip["out_proj_W"]).astype(np.float32)
    OPB = (np.diag(ip["out_norm_b"]) @ ip["out_proj_W"]).astype(np.float32)
    col = lambda v: np.ascontiguousarray(v.reshape(-1, 1), dtype=np.float32)
    cbw = ip["cb_dw_W"].reshape(HID, 9)
    base = dict(OPm=_bf(OPm), OPB=_bf(OPB),
                PW1=_bf(ip["cb_pw1_W"][:, :, 0, 0].T),
                g1=col(ip["cb_bn1_g"]), bb1=col(ip["cb_bn1_b"]),
                cbdiag0=_bf(_diag9(cbw[0:DT0], DT0)),
                cbdiag1=_bf(_diag9(cbw[DT0:HID], DT1)),
                g2=col(ip["cb_bn2_g"]), bb2=col(ip["cb_bn2_b"]),
                PW2=_bf(ip["cb_pw2_W"][:, :, 0, 0].T),
                g3=col(ip["cb_bn3_g"]), bb3=col(ip["cb_bn3_b"]),
                fw=col(ip["norm_w"]), fb=col(ip["norm_b"]))
    maps = []
    for c in range(8):
        b = c // 2
        m = dict(base)
        ymf = (np.asarray(res1[2 * b]["oq"], np.float32)
               + np.asarray(res1[2 * b + 1]["oq"], np.float32))
        m["ym"] = _bf(ymf)
        m["xin"] = np.asarray(res1[2 * b]["ox"], np.float32)
        m["zin"] = np.ascontiguousarray(res1[2 * b]["oz"])
        maps.append(m)
    return maps


def kernel(**inputs):
    ip = {k: np.asarray(v, np.float32) for k, v in inputs.items()}
    nc1, nc2 = _get_ncs()
    res1 = run_bass_kernel_spmd(nc1, prep_pass1(ip), list(range(8))).results
    res2 = run_bass_kernel_spmd(nc2, prep_pass2(ip, res1), list(range(8))).results
    outs = [np.asarray(res2[2 * b]["o"], np.float32).T.reshape(H_, W_, COUT)
            for b in range(B_)]
    return np.stack(outs).astype(np.float32)


# revision 32
# speedup vs baseline: 1.7190x; 1.0050x over previous
"""Trainium2 Bass kernel for nn_DecoderFusionBlock (VSS/Mamba decoder fusion block).

Two-pass SPMD over 8 cores:
  pass 1: core c -> batch b=c//2, plane=c%2 (row-/col-major spatial order).
          proj/LN/in_proj (f32r / bf16 matmuls), depthwise conv via PE diag
          matmuls, then the selective scan for the plane's two directions.
          bf16 data path with fp32 scan state; B/C broadcast to all channel
          partitions via a DRAM-staged broadcast DMA so the big elementwise
          multiplies run in the DVE 2x (2-byte) mode; the n-state reduction
          runs on the PE as identity-weight matmul accumulation in PSUM.
  host:   ym[b] = Q[2b] + Q[2b+1]  (the only cross-core reduction)
  pass 2: core c -> batch b=c//2: out-norm, gate, out_proj+residual,
          ConvBlock (conv again via PE), final LayerNorm.
"""

import contextlib
import numpy as np

import concourse.bass as bass
import concourse.tile as tile
from concourse import bacc, mybir
from concourse.bass_utils import run_bass_kernel_spmd

f32 = mybir.dt.float32
f32r = mybir.dt.float32r
bf16 = mybir.dt.bfloat16
AF = mybir.ActivationFunctionType
OP_ = mybir.AluOpType

B_, H_, W_ = 4, 48, 48
L = H_ * W_
CIN, COUT = 192, 96
DIN, NST, RNK, KDIR = 192, 16, 6, 4
HID = 192
EPS = 1e-5
DT0, DT1 = 128, 64
MMC = 512
MM = [(s, min(MMC, L - s)) for s in range(0, L, MMC)]
LC = 256
SC = [(i * LC, LC) for i in range(L // LC)]
CROWS = [(0, 10), (10, 10), (20, 10), (30, 10), (40, 8)]


def _rev(ap, s, w):
    hi = L - 1 - s
    lo = hi - w
    return ap[:, hi::-1] if lo < 0 else ap[:, hi:lo:-1]


def _sl(ap, k, s, w):
    return ap[:, s:s + w] if k == 0 else _rev(ap, s, w)


def _rep(a, n):
    return bass.AP(tensor=a.tensor, offset=a.offset, ap=[a.ap[0], [0, n], a.ap[1]])


def _twh(a):
    st = a.ap[1][0]
    return bass.AP(tensor=a.tensor, offset=a.offset,
                   ap=[a.ap[0], [st, 48], [48 * st, 48]])


def _pl3(a):
    st = a.ap[1][0]
    return bass.AP(tensor=a.tensor, offset=a.offset,
                   ap=[a.ap[0], [48 * st, 48], [st, 48]])


# ---------------------------------------------------------------- pass 1
def build_nc1():
    nc = bacc.Bacc("TRN2", target_bir_lowering=False, debug=False, num_devices=8)
    din = {}

    def I(name, shape, dt=f32):
        din[name] = nc.dram_tensor(name, shape, dt, kind="ExternalInput")

    I("xc_t", [CIN, L], f32r)
    I("projW", [CIN, COUT], f32r); I("projb", [COUT, 1])
    I("W1", [COUT, 2 * DIN], bf16); I("b1", [2 * DIN, 1])
    I("cdiag0", [DT0, 9, DT0], bf16); I("cdiag1", [DT1, 9, DT1], bf16)
    I("convb", [DIN, 1])
    I("eye", [128, 128], bf16)
    I("xpw", [2, DIN, RNK + 2 * NST], bf16)
    I("dtw", [2, RNK, DIN], bf16)
    I("dtb", [2, DIN, 1]); I("acoef", [2, DIN, NST]); I("dsum", [DIN, 1])
    I("mrow", [DIN, 1]); I("mcol", [DIN, 1])
    oq_d = nc.dram_tensor("oq", [DIN, L], bf16, kind="ExternalOutput")
    ox_d = nc.dram_tensor("ox", [COUT, L], f32, kind="ExternalOutput")
    oz_d = nc.dram_tensor("oz", [DIN, L], bf16, kind="ExternalOutput")
    bcd = nc.dram_tensor("BCd", [2, 32, L], bf16, kind="Internal")

    ctx = contextlib.ExitStack()
    with tile.TileContext(nc) as tc, ctx:
        const = ctx.enter_context(tc.tile_pool(name="const", bufs=1))
        big = ctx.enter_context(tc.tile_pool(name="big", bufs=1))
        work = ctx.enter_context(tc.tile_pool(name="work", bufs=2))
        scn = ctx.enter_context(tc.tile_pool(name="scn", bufs=1))
        psM = ctx.enter_context(tc.tile_pool(name="psM", bufs=2, space="PSUM"))
        psY = ctx.enter_context(tc.tile_pool(name="psY", bufs=2, space="PSUM"))

        def load2(name, rows, cols, dt=f32):
            t0 = const.tile([DT0, cols], dt, tag=name + "0", name=name + "0")
            t1 = const.tile([DT1, cols], dt, tag=name + "1", name=name + "1")
            nc.sync.dma_start(t0[:], din[name][0:DT0])
            nc.sync.dma_start(t1[:], din[name][DT0:rows])
            return t0, t1

        projW0 = const.tile([DT0, COUT], f32r)
        projW1 = const.tile([DT1, COUT], f32r)
        nc.sync.dma_start(projW0[:], din["projW"][0:DT0])
        nc.sync.dma_start(projW1[:], din["projW"][DT0:CIN])
        projb = const.tile([COUT, 1], f32)
        nc.sync.dma_start(projb[:], din["projb"][:])
        W1t = const.tile([COUT, 2 * DIN], bf16)
        nc.sync.dma_start(W1t[:], din["W1"][:])
        b1x0 = const.tile([DT0, 1], f32); nc.sync.dma_start(b1x0[:], din["b1"][0:128])
        b1x1 = const.tile([DT1, 1], f32); nc.sync.dma_start(b1x1[:], din["b1"][128:192])
        b1z0 = const.tile([DT0, 1], f32); nc.sync.dma_start(b1z0[:], din["b1"][192:320])
        b1z1 = const.tile([DT1, 1], f32); nc.sync.dma_start(b1z1[:], din["b1"][320:384])
        cdiag0 = const.tile([DT0, 9, DT0], bf16)
        nc.sync.dma_start(cdiag0[:], din["cdiag0"][:])
        cdiag1 = const.tile([DT1, 9, DT1], bf16)
        nc.sync.dma_start(cdiag1[:], din["cdiag1"][:])
        convb0, convb1 = load2("convb", DIN, 1)
        eye = const.tile([128, 128], bf16)
        nc.sync.dma_start(eye[:], din["eye"][:])
        dsum0, dsum1 = load2("dsum", DIN, 1)
        mrow0, mrow1 = load2("mrow", DIN, 1)
        mcol0, mcol1 = load2("mcol", DIN, 1)
        kw = []
        for k in range(2):
            xp0 = const.tile([DT0, RNK + 2 * NST], bf16, name=f"xp{k}0")
            xp1 = const.tile([DT1, RNK + 2 * NST], bf16, name=f"xp{k}1")
            nc.sync.dma_start(xp0[:], din["xpw"][k, 0:DT0])
            nc.sync.dma_start(xp1[:], din["xpw"][k, DT0:DIN])
            dtw = const.tile([38, DIN], bf16, tag="dtwm", name=f"dtw{k}",
                             bufs=1) if k == 0 else kw[0]["dtwt"]
            nc.sync.dma_start(dtw[k * 32:k * 32 + RNK], din["dtw"][k])
            dtb0 = const.tile([DT0, 1], f32, name=f"dtb{k}0")
            dtb1 = const.tile([DT1, 1], f32, name=f"dtb{k}1")
            nc.sync.dma_start(dtb0[:], din["dtb"][k, 0:DT0])
            nc.sync.dma_start(dtb1[:], din["dtb"][k, DT0:DIN])
            ac0 = const.tile([DT0, NST], f32, name=f"ac{k}0")
            ac1 = const.tile([DT1, NST], f32, name=f"ac{k}1")
            nc.sync.dma_start(ac0[:], din["acoef"][k, 0:DT0])
            nc.sync.dma_start(ac1[:], din["acoef"][k, DT0:DIN])
            kw.append(dict(xp=(xp0, xp1), dtwt=dtw,
                           dtw=dtw[k * 32:k * 32 + RNK], dtb=(dtb0, dtb1),
                           ac=(ac0, ac1)))

        ones128 = const.tile([128, 1], f32); nc.vector.memset(ones128[:], 1.0)
        onesrow = const.tile([1, 128], bf16); nc.vector.memset(onesrow[:], 1.0)
        epsc = const.tile([1, 1], f32); nc.vector.memset(epsc[:], EPS)

        # ---- load + proj (f32r matmuls, x_t kept fp32 for residual) ----
        xc0 = big.tile([DT0, L], f32r, tag="xc0")
        xc1 = big.tile([DT1, L], f32r, tag="xc1")
        nc.sync.dma_start(xc0[:], din["xc_t"][0:DT0])
        nc.sync.dma_start(xc1[:], din["xc_t"][DT0:CIN])
        x_t = big.tile([COUT, L], f32, tag="x_t")
        for (s, w) in MM:
            ps = psM.tile([128, MMC], f32, tag="mm", name="psproj")
            nc.tensor.matmul(ps[:COUT, :w], projW0[:], xc0[:, s:s + w],
                             start=True, stop=False)
            nc.tensor.matmul(ps[:COUT, :w], projW1[:], xc1[:, s:s + w],
                             start=False, stop=True)
            nc.scalar.activation(x_t[:, s:s + w], ps[:COUT, :w], AF.Identity,
                                 bias=projb[:])
        nc.sync.dma_start(ox_d[:], x_t[:])

        # ---- LN1 (Copy + Sqrt share the act-table phase) -> xn bf16 ----
        xn_t = big.tile([COUT, L], bf16, tag="xn")
        for (s, w) in MM:
            ps1 = psM.tile([128, MMC], f32, tag="mm", name="pss1")
            nc.tensor.matmul(ps1[:1, :w], ones128[:COUT], x_t[:, s:s + w],
                             start=True, stop=True)
            mrw = work.tile([1, MMC], bf16, tag="mrw", bufs=1)
            nc.scalar.activation(mrw[:, :w], ps1[:1, :w], AF.Copy, scale=1.0 / COUT)
            sq = work.tile([128, MMC], f32, tag="sqc", bufs=1)
            nc.vector.tensor_tensor(out=sq[:COUT, :w], in0=x_t[:, s:s + w],
                                    in1=x_t[:, s:s + w], op=OP_.mult)
            ps2 = psM.tile([128, MMC], f32, tag="mm", name="pss2")
            nc.tensor.matmul(ps2[:1, :w], ones128[:COUT], sq[:COUT, :w],
                             start=True, stop=True)
            mq = work.tile([1, MMC], f32, tag="mq", bufs=1)
            nc.scalar.activation(mq[:, :w], ps2[:1, :w], AF.Copy, scale=1.0 / COUT)
            msq = work.tile([1, MMC], f32, tag="msq", bufs=1)
            nc.vector.tensor_tensor(out=msq[:, :w], in0=mrw[:, :w],
                                    in1=mrw[:, :w], op=OP_.mult)
            nc.vector.tensor_tensor(out=mq[:, :w], in0=mq[:, :w],
                                    in1=msq[:, :w], op=OP_.subtract)
            nc.scalar.activation(mq[:, :w], mq[:, :w], AF.Sqrt, bias=epsc[:])
            rsw = work.tile([1, MMC], bf16, tag="rsw", bufs=1)
            with nc.allow_low_precision(reason="bf16 1/std is well conditioned"):
                nc.vector.reciprocal(rsw[:, :w], mq[:, :w])
            pm = psM.tile([128, MMC], f32, tag="mm", name="psbm")
            nc.tensor.matmul(pm[:, :w], onesrow[:], mrw[:, :w],
                             start=True, stop=True)
            pr = psM.tile([128, MMC], f32, tag="mm", name="psbr")
            nc.tensor.matmul(pr[:, :w], onesrow[:], rsw[:, :w],
                             start=True, stop=True)
            xn_ = work.tile([128, MMC], bf16, tag="xn_", bufs=1)
            nc.vector.tensor_tensor(out=xn_[:COUT, :w], in0=x_t[:, s:s + w],
                                    in1=pm[:COUT, :w], op=OP_.subtract)
            nc.vector.tensor_tensor(out=xn_t[:, s:s + w], in0=xn_[:COUT, :w],
                                    in1=pr[:COUT, :w], op=OP_.mult)

        # ---- in_proj (xm tiles bf16; z silu'd -> DRAM bf16) ----
        xm0 = big.tile([DT0, L], bf16, tag="xm0")
        xm1 = big.tile([DT1, L], bf16, tag="xm1")
        for (s, w) in MM:
            for (coff, rows, bcol, dst, zoff) in (
                    (0, DT0, b1x0, xm0, None), (DT0, DT1, b1x1, xm1, None),
                    (DIN, DT0, b1z0, None, 0), (DIN + DT0, DT1, b1z1, None, DT0)):
                psi = psM.tile([128, MMC], f32, tag="mm", name="psip")
                nc.tensor.matmul(psi[:rows, :w], W1t[:, coff:coff + rows],
                                 xn_t[:, s:s + w], start=True, stop=True)
                if dst is not None:
                    nc.scalar.activation(dst[:, s:s + w], psi[:rows, :w],
                                         AF.Identity, bias=bcol[:])
                else:
                    zc = work.tile([128, MMC], bf16, tag="zc", bufs=1)
                    nc.scalar.activation(zc[:rows, :w], psi[:rows, :w], AF.Silu,
                                         bias=bcol[:])
                    nc.sync.dma_start(oz_d[zoff:zoff + rows, s:s + w], zc[:rows, :w])

        # ---- depthwise conv via PE diag matmuls + fused SiLU; the
        #      transposed copy for the plane transform happens per row-chunk
        xs0 = big.tile([DT0, L], bf16, tag="xs0")
        xs1 = big.tile([DT1, L], bf16, tag="xs1")
        xt0 = big.tile([DT0, L], bf16, tag="xm0", name="xt0")
        xt1 = big.tile([DT1, L], bf16, tag="xm1", name="xt1")
        for (src, cd, rows, out, bcol, xtt, teng) in (
                (xm0, cdiag0, DT0, xs0, convb0, xt0, nc.vector),
                (xm1, cdiag1, DT1, xs1, convb1, xt1, nc.vector)):
            pad = work.tile([128, 50, 50], bf16, tag="pad", bufs=1)
            nc.vector.memset(pad[:rows], 0.0)
            nc.vector.tensor_copy(out=pad[:rows, 1:49, 1:49], in_=_pl3(src[:]))
            for (y0, ny) in CROWS:
                pc = psM.tile([128, MMC], f32, tag="mm", name="pscv")
                for j in range(9):
                    dy, dx = divmod(j, 3)
                    view = pad[:rows, y0 + dy:y0 + dy + ny, dx:dx + 48]
                    nc.tensor.matmul(pc[:rows, :ny * 48], cd[:, j], view,
                                     start=(j == 0), stop=(j == 8))
                nc.scalar.activation(out[:, y0 * 48:(y0 + ny) * 48],
                                     pc[:rows, :ny * 48], AF.Silu, bias=bcol[:])
                # xt[p, x, y] = xs[p, y, x] for this y-chunk
                xin = _pl3(out[:])[:, y0:y0 + ny, :]
                xout = bass.AP(tensor=xtt.tensor, offset=xtt[:].offset + y0,
                               ap=[xtt[:].ap[0], [1, ny], [48, 48]])
                teng.tensor_copy(out=xout, in_=xin)
        nc.vector.tensor_scalar_mul(xs0[:], xs0[:], mrow0[:])
        nc.vector.scalar_tensor_tensor(out=xs0[:], in0=xt0[:], scalar=mcol0[:],
                                       in1=xs0[:], op0=OP_.mult, op1=OP_.add)
        nc.vector.tensor_scalar_mul(xs1[:], xs1[:], mrow1[:])
        nc.vector.scalar_tensor_tensor(out=xs1[:], in0=xt1[:], scalar=mcol1[:],
                                       in1=xs1[:], op0=OP_.mult, op1=OP_.add)

        # ---- U96: x_dbl for both k; B/C staged to DRAM bf16 ----
        u96m = big.tile([38, L], bf16, tag="u96r")
        u96rs = (u96m[0:RNK], u96m[32:38])
        for k in range(2):
            W = kw[k]
            rb = k * 32          # rank section base: 0 (k0) / 32 (k1)
            bb = rb + 32
            cb = 64 if k == 0 else 0
            for (s, w) in MM:
                ps = psM.tile([128, MMC], f32, tag="mm", name="psU")
                for (coff, ubase, m) in ((0, rb, RNK), (RNK, bb, NST),
                                         (RNK + NST, cb, NST)):
                    nc.tensor.matmul(ps[ubase:ubase + m, :w],
                                     W["xp"][0][:, coff:coff + m],
                                     xs0[:, s:s + w], start=True, stop=False)
                    nc.tensor.matmul(ps[ubase:ubase + m, :w],
                                     W["xp"][1][:, coff:coff + m],
                                     xs1[:, s:s + w], start=False, stop=True)
                nc.scalar.activation(u96rs[k][:, s:s + w],
                                     ps[rb:rb + RNK, :w], AF.Copy)
                bcsw = work.tile([112, MMC], bf16, tag="bcsw")
                nc.scalar.activation(bcsw[bb:bb + NST, :w], ps[bb:bb + NST, :w],
                                     AF.Copy)
                nc.scalar.activation(bcsw[cb:cb + NST, :w], ps[cb:cb + NST, :w],
                                     AF.Copy)
                nc.sync.dma_start(bcd[k, 0:NST, s:s + w], bcsw[bb:bb + NST, :w])
                nc.sync.dma_start(bcd[k, NST:32, s:s + w], bcsw[cb:cb + NST, :w])

        # ---- delta (Softplus) for both k, both dt ----
        dcs = []
        for k in range(2):
            W = kw[k]
            dc0 = big.tile([DT0, L], bf16, tag="xc0" if k == 0 else "xc1",
                           name=f"dc{k}0")
            dc1 = big.tile([DT1, L], bf16, tag=f"dc{k}1", name=f"dc{k}1")
            for (s, w) in MM:
                for dt, (rows, dct) in enumerate(((DT0, dc0), (DT1, dc1))):
                    ps = psM.tile([128, MMC], f32, tag="mm", name="psdt")
                    nc.tensor.matmul(ps[:rows, :w],
                                     W["dtw"][:, dt * DT0:dt * DT0 + rows],
                                     u96rs[k][:, s:s + w],
                                     start=True, stop=True)
                    nc.scalar.activation(dct[:, s:s + w], ps[:rows, :w],
                                         AF.Exp, bias=W["dtb"][dt][:])
            # in-place Ln: dc = ln(1 + e^u) = softplus(u), batched per k
            for (s, w) in MM:
                for dct in (dc0, dc1):
                    nc.scalar.activation(dct[:, s:s + w], dct[:, s:s + w],
                                         AF.Ln, bias=1.0)
            dcs.append((dc0, dc1))

        # ---- P init with the direction-independent D term ----
        P0 = big.tile([DT0, L], bf16, tag="xm0", name="P0")
        P1 = big.tile([DT1, L], bf16, tag="xt1m", name="P1")
        nc.vector.tensor_scalar_mul(P0[:], xs0[:], dsum0[:])
        nc.gpsimd.tensor_scalar_mul(P1[:], xs1[:], dsum1[:])

        # ---- selective scan ----
        hp = [[scn.tile([128, NST, 1], bf16, tag=f"hp{k}{dt}", name=f"hp{k}{dt}")
               for dt in range(2)] for k in range(2)]
        for k in range(2):
            for dt in range(2):
                nc.vector.memset(hp[k][dt][:], 0.0)

        pairs = [(c, k) for c in range(len(SC)) for k in range(2)]
        bcr_tiles = {}

        def issue_bcast(i):
            c, k = pairs[i]
            s, w = SC[c]
            s0 = s if k == 0 else L - s - w
            t = scn.tile([128, 32, LC], bf16, tag="bcr", bufs=3,
                         name=f"bcr{i}")
            src = bass.AP(tensor=bcd, offset=k * 32 * L + s0,
                          ap=[[0, 128], [L, 32], [1, w]])
            nc.sync.dma_start(t[:], src)
            bcr_tiles[i] = t

        def tail(rows, dA, Ht, crep, hpt, pdst):
            # after the scan: save carry, G = H*C, PE n-reduction, P +=
            nc.gpsimd.tensor_copy(out=hpt[:rows], in_=Ht[:rows, :, LC:LC + 1])
            nc.vector.tensor_tensor(out=dA[:rows, :, 1:], in0=Ht[:rows, :, 1:],
                                    in1=crep[:rows], op=OP_.mult)
            psy = psY.tile([128, LC], f32, tag="psy", name="psy")
            for n in range(NST):
                nc.tensor.matmul(psy[:rows, :], eye[:rows, :rows],
                                 dA[:rows, n, 1:],
                                 start=(n == 0), stop=(n == NST - 1))
            nc.vector.tensor_tensor(out=pdst, in0=psy[:rows, :], in1=pdst,
                                    op=OP_.add)

        pending = []
        issue_bcast(0)
        for i, (c, k) in enumerate(pairs):
            s, w = SC[c]
            W = kw[k]
            bcr = bcr_tiles.pop(i)
            if k == 0:
                brep = bcr[:, 0:NST, :]
                crep = bcr[:, NST:32, :]
            else:
                brep = bcr[:, 0:NST, ::-1]
                crep = bcr[:, NST:32, ::-1]
            for dt, (rows, xst) in enumerate(((DT0, xs0), (DT1, xs1))):
                if len(pending) >= 3:
                    # flush the 3-iterations-old deferred tail before its
                    # dA/dBu/Ht buffer slots are reused below
                    pending.pop(0)()
                if dt == 0 and i + 1 < len(pairs):
                    # safe point: all readers of bcr slot (i+1)%3's previous
                    # occupant (pair i-2) have been emitted by now
                    issue_bcast(i + 1)
                dct = dcs[k][dt]
                dcsl = _sl(dct[:], k, s, w)
                xssl = _sl(xst[:], k, s, w)
                dxc = work.tile([128, LC], bf16, tag=f"dxc{dt}",
                                name=f"dxc{dt}", bufs=2)
                nc.gpsimd.tensor_tensor(out=dxc[:rows, :], in0=dcsl, in1=xssl,
                                        op=OP_.mult)
                dA = scn.tile([128, NST, LC + 1], bf16, tag="dA",
                              name=f"dA{dt}", bufs=3)
                dBu = scn.tile([128, NST, LC + 1], bf16, tag="dBu",
                               name=f"dBu{dt}", bufs=2)
                Ht = scn.tile([128, NST, LC + 1], bf16, tag="Ht",
                              name=f"Ht{dt}", bufs=3)
                nc.gpsimd.memset(dA[:rows, :, 0:1], 0.0)
                nc.gpsimd.tensor_copy(out=dBu[:rows, :, 0:1], in_=hp[k][dt][:rows])
                for n in range(NST):
                    nc.scalar.activation(dA[:rows, n, 1:], dcsl, AF.Exp,
                                         scale=W["ac"][dt][:, n:n + 1])
                beng = nc.vector if dt == 0 else nc.gpsimd
                beng.tensor_tensor(out=dBu[:rows, :, 1:],
                                   in0=_rep(dxc[:rows, :], NST),
                                   in1=brep[:rows], op=OP_.mult)
                nc.vector.tensor_tensor_scan(
                    out=Ht[:rows].rearrange("p a b -> p (a b)"),
                    data0=dA[:rows].rearrange("p a b -> p (a b)"),
                    data1=dBu[:rows].rearrange("p a b -> p (a b)"),
                    initial=0.0, op0=OP_.mult, op1=OP_.add)
                Pt = P0 if dt == 0 else P1
                args = (rows, dA, Ht, crep, hp[k][dt], _sl(Pt[:], k, s, w))
                pending.append(lambda a=args: tail(*a))
        for fn in pending:
            fn()

        # ---- Q = mrow*P + mcol*transpose(P) ----
        Q0 = big.tile([DT0, L], bf16, tag="xs0", name="Q0")
        Q1 = big.tile([DT1, L], bf16, tag="xs1", name="Q1")
        nc.vector.tensor_scalar_mul(Q0[:], _twh(P0[:]), mcol0[:])
        nc.vector.scalar_tensor_tensor(out=Q0[:], in0=P0[:], scalar=mrow0[:],
                                       in1=Q0[:], op0=OP_.mult, op1=OP_.add)
        nc.gpsimd.tensor_scalar_mul(Q1[:], _twh(P1[:]), mcol1[:])
        qtm = big.tile([DT1, L], bf16, tag="xm1", name="qtm")
        nc.gpsimd.tensor_scalar_mul(qtm[:], P1[:], mrow1[:])
        nc.gpsimd.tensor_tensor(out=Q1[:], in0=qtm[:], in1=Q1[:], op=OP_.add)
        nc.sync.dma_start(oq_d[0:DT0], Q0[:])
        nc.sync.dma_start(oq_d[DT0:DIN], Q1[:])
    nc.compile()
    return nc


# ---------------------------------------------------------------- pass 2
def build_nc2():
    nc = bacc.Bacc("TRN2", target_bir_lowering=False, debug=False, num_devices=8)
    din = {}

    def I(name, shape, dt=f32):
        din[name] = nc.dram_tensor(name, shape, dt, kind="ExternalInput")

    I("ym", [DIN, L], bf16); I("xin", [COUT, L]); I("zin", [DIN, L], bf16)
    I("OPm", [DIN, COUT], bf16); I("OPB", [DIN, COUT], bf16)
    I("PW1", [COUT, HID], bf16); I("g1", [HID, 1]); I("bb1", [HID, 1])
    I("cbdiag0", [DT0, 9, DT0], bf16); I("cbdiag1", [DT1, 9, DT1], bf16)
    I("g2", [HID, 1]); I("bb2", [HID, 1])
    I("PW2", [HID, COUT], bf16); I("g3", [COUT, 1]); I("bb3", [COUT, 1])
    I("fw", [COUT, 1]); I("fb", [COUT, 1])
    out_d = nc.dram_tensor("o", [COUT, L], f32, kind="ExternalOutput")

    ctx = contextlib.ExitStack()
    with tile.TileContext(nc) as tc, ctx:
        const = ctx.enter_context(tc.tile_pool(name="const", bufs=1))
        big = ctx.enter_context(tc.tile_pool(name="big", bufs=1))
        work = ctx.enter_context(tc.tile_pool(name="work", bufs=2))
        psM = ctx.enter_context(tc.tile_pool(name="psM", bufs=2, space="PSUM"))

        def load2(name, rows, cols, dt=f32):
            t0 = const.tile([DT0, cols], dt, tag=name + "0", name=name + "0")
            t1 = const.tile([DT1, cols], dt, tag=name + "1", name=name + "1")
            nc.sync.dma_start(t0[:], din[name][0:DT0])
            nc.sync.dma_start(t1[:], din[name][DT0:rows])
            return t0, t1

        def load1(name, rows):
            t = const.tile([rows, 1], f32, tag=name, name=name)
            nc.sync.dma_start(t[:], din[name][:])
            return t

        OP0, OP1 = load2("OPm", DIN, COUT, bf16)
        OPB0, OPB1 = load2("OPB", DIN, COUT, bf16)
        PW1t = const.tile([COUT, HID], bf16)
        nc.sync.dma_start(PW1t[:], din["PW1"][:])
        g1c0, g1c1 = load2("g1", HID, 1)
        bb1c0, bb1c1 = load2("bb1", HID, 1)
        cbd0 = const.tile([DT0, 9, DT0], bf16)
        nc.sync.dma_start(cbd0[:], din["cbdiag0"][:])
        cbd1 = const.tile([DT1, 9, DT1], bf16)
        nc.sync.dma_start(cbd1[:], din["cbdiag1"][:])
        g2c0, g2c1 = load2("g2", HID, 1)
        bb2c0, bb2c1 = load2("bb2", HID, 1)
        PW20, PW21 = load2("PW2", HID, COUT, bf16)
        g3c = load1("g3", COUT); bb3c = load1("bb3", COUT)
        fwc = load1("fw", COUT); fbc = load1("fb", COUT)
        onesb = const.tile([128, 1], bf16); nc.vector.memset(onesb[:], 1.0)
        onesrow = const.tile([1, 128], bf16); nc.vector.memset(onesrow[:], 1.0)
        epsc = const.tile([1, 1], f32); nc.vector.memset(epsc[:], EPS)

        ym0 = big.tile([DT0, L], bf16, tag="ym0")
        ym1 = big.tile([DT1, L], bf16, tag="ym1")
        nc.sync.dma_start(ym0[:], din["ym"][0:DT0])
        nc.sync.dma_start(ym1[:], din["ym"][DT0:DIN])
        xres = big.tile([COUT, L], f32, tag="xres")
        nc.sync.dma_start(xres[:], din["xin"][:])
        zc0 = big.tile([DT0, L], bf16, tag="zc0")
        zc1 = big.tile([DT1, L], bf16, tag="zc1")
        nc.sync.dma_start(zc0[:], din["zin"][0:DT0])
        nc.sync.dma_start(zc1[:], din["zin"][DT0:DIN])

        # out-norm stats over 192 partitions (batched, then one Rsqrt)
        mean_r = big.tile([1, L], bf16, tag="mean")
        rs_r = big.tile([1, L], bf16, tag="rs")
        for (s, w) in MM:
            ps = psM.tile([128, MMC], f32, tag="mm", name="pso1")
            nc.tensor.matmul(ps[:1, :w], onesb[:], ym0[:, s:s + w],
                             start=True, stop=False)
            nc.tensor.matmul(ps[:1, :w], onesb[:DT1], ym1[:, s:s + w],
                             start=False, stop=True)
            nc.scalar.activation(mean_r[:, s:s + w], ps[:1, :w], AF.Copy,
                                 scale=1.0 / DIN)
            ps2 = psM.tile([128, MMC], f32, tag="mm", name="pso2")
            for i, (t, rows) in enumerate(((ym0, DT0), (ym1, DT1))):
                sq = work.tile([128, MMC], bf16, tag="sqc", bufs=1)
                nc.vector.tensor_tensor(out=sq[:rows, :w], in0=t[:, s:s + w],
                                        in1=t[:, s:s + w], op=OP_.mult)
                nc.tensor.matmul(ps2[:1, :w], onesb[:rows], sq[:rows, :w],
                                 start=(i == 0), stop=(i == 1))
            mq = work.tile([1, MMC], f32, tag="mq", bufs=1)
            nc.scalar.activation(mq[:, :w], ps2[:1, :w], AF.Copy,
                                 scale=1.0 / DIN)
            msqc = work.tile([1, MMC], f32, tag="msqc", bufs=1)
            nc.vector.tensor_tensor(out=msqc[:, :w], in0=mean_r[:, s:s + w],
                                    in1=mean_r[:, s:s + w], op=OP_.mult)
            nc.vector.tensor_tensor(out=mq[:, :w], in0=mq[:, :w],
                                    in1=msqc[:, :w], op=OP_.subtract)
            nc.scalar.activation(mq[:, :w], mq[:, :w], AF.Sqrt, bias=epsc[:])
            with nc.allow_low_precision(reason="bf16 1/std is well conditioned"):
                nc.vector.reciprocal(rs_r[:, s:s + w], mq[:, :w])

        x2f = big.tile([COUT, L], f32, tag="x2f")
        x2b = big.tile([COUT, L], bf16, tag="x2b")
        for (s, w) in MM:
            pm = psM.tile([128, MMC], f32, tag="mm", name="psm")
            nc.tensor.matmul(pm[:, :w], onesrow[:], mean_r[:, s:s + w],
                             start=True, stop=True)
            pr = psM.tile([128, MMC], f32, tag="mm", name="psr")
            nc.tensor.matmul(pr[:, :w], onesrow[:], rs_r[:, s:s + w],
                             start=True, stop=True)
            po = psM.tile([128, MMC], f32, tag="mm", name="pso")
            for i, (t, z, rows) in enumerate(((ym0, zc0, DT0), (ym1, zc1, DT1))):
                yn = work.tile([128, MMC], bf16, tag=f"yn{i}", name=f"yn{i}")
                nc.vector.tensor_tensor(out=yn[:rows, :w], in0=t[:, s:s + w],
                                        in1=pm[:rows, :w], op=OP_.subtract)
                nc.vector.tensor_tensor(out=yn[:rows, :w], in0=yn[:rows, :w],
                                        in1=pr[:rows, :w], op=OP_.mult)
                nc.vector.tensor_tensor(out=yn[:rows, :w], in0=yn[:rows, :w],
                                        in1=z[:, s:s + w], op=OP_.mult)
                OPt = OP0 if i == 0 else OP1
                OPBt = OPB0 if i == 0 else OPB1
                nc.tensor.matmul(po[:COUT, :w], OPt[:], yn[:rows, :w],
                                 start=(i == 0), stop=False)
                nc.tensor.matmul(po[:COUT, :w], OPBt[:], z[:, s:s + w],
                                 start=False, stop=(i == 1))
            nc.vector.tensor_tensor(out=x2f[:, s:s + w], in0=po[:COUT, :w],
                                    in1=xres[:, s:s + w], op=OP_.add)
            nc.scalar.activation(x2b[:, s:s + w], x2f[:, s:s + w], AF.Copy)

        # ConvBlock: PW1 + gelu
        t0 = big.tile([DT0, L], bf16, tag="ym0", name="t0")
        t1 = big.tile([DT1, L], bf16, tag="ym1", name="t1")
        for (s, w) in MM:
            for (dst, coff, rows, gc_, bc_) in ((t0, 0, DT0, g1c0, bb1c0),
                                                (t1, DT0, DT1, g1c1, bb1c1)):
                ps = psM.tile([128, MMC], f32, tag="mm", name="psp1")
                nc.tensor.matmul(ps[:rows, :w], PW1t[:, coff:coff + rows],
                                 x2b[:, s:s + w], start=True, stop=True)
                nc.scalar.activation(dst[:, s:s + w], ps[:rows, :w], AF.Gelu,
                                     bias=bc_[:], scale=gc_[:])
        # dw conv via PE; fused bn2+gelu on psum
        v0 = big.tile([DT0, L], bf16, tag="zc0", name="v0")
        v1 = big.tile([DT1, L], bf16, tag="zc1", name="v1")
        for (src, cd, rows, out, gc_, bc_) in (
                (t0, cbd0, DT0, v0, g2c0, bb2c0),
                (t1, cbd1, DT1, v1, g2c1, bb2c1)):
            pad = work.tile([128, 50, 50], bf16, tag="pad", bufs=1)
            nc.vector.memset(pad[:rows], 0.0)
            nc.vector.tensor_copy(out=pad[:rows, 1:49, 1:49], in_=_pl3(src[:]))
            for (y0, ny) in CROWS:
                pc = psM.tile([128, MMC], f32, tag="mm", name="pscv")
                for j in range(9):
                    dy, dx = divmod(j, 3)
                    view = pad[:rows, y0 + dy:y0 + dy + ny, dx:dx + 48]
                    nc.tensor.matmul(pc[:rows, :ny * 48], cd[:, j], view,
                                     start=(j == 0), stop=(j == 8))
                nc.scalar.activation(out[:, y0 * 48:(y0 + ny) * 48],
                                     pc[:rows, :ny * 48], AF.Gelu,
                                     bias=bc_[:], scale=gc_[:])
        # PW2 + bn3 + residual
        x3f = big.tile([COUT, L], f32, tag="x3f")
        x3b = big.tile([COUT, L], bf16, tag="xres", name="x3b")
        for (s, w) in MM:
            ps = psM.tile([128, MMC], f32, tag="mm", name="psp2")
            nc.tensor.matmul(ps[:COUT, :w], PW20[:], v0[:, s:s + w],
                             start=True, stop=False)
            nc.tensor.matmul(ps[:COUT, :w], PW21[:], v1[:, s:s + w],
                             start=False, stop=True)
            cbt = work.tile([128, MMC], bf16, tag="cbt", bufs=1)
            nc.scalar.activation(cbt[:COUT, :w], ps[:COUT, :w], AF.Identity,
                                 bias=bb3c[:], scale=g3c[:])
            nc.vector.tensor_tensor(out=x3f[:, s:s + w], in0=cbt[:COUT, :w],
                                    in1=x2f[:, s:s + w], op=OP_.add)
            nc.scalar.activation(x3b[:, s:s + w], x3f[:, s:s + w], AF.Copy)

        # final LN
        mean2 = big.tile([1, L], bf16, tag="mean2")
        rs2 = big.tile([1, L], bf16, tag="rs2")
        for (s, w) in MM:
            ps = psM.tile([128, MMC], f32, tag="mm", name="psf1")
            nc.tensor.matmul(ps[:1, :w], onesb[:COUT], x3b[:, s:s + w],
                             start=True, stop=True)
            nc.scalar.activation(mean2[:, s:s + w], ps[:1, :w], AF.Copy,
                                 scale=1.0 / COUT)
            sq = work.tile([128, MMC], bf16, tag="sqc", bufs=1)
            nc.vector.tensor_tensor(out=sq[:COUT, :w], in0=x3b[:, s:s + w],
                                    in1=x3b[:, s:s + w], op=OP_.mult)
            ps2 = psM.tile([128, MMC], f32, tag="mm", name="psf2")
            nc.tensor.matmul(ps2[:1, :w], onesb[:COUT], sq[:COUT, :w],
                             start=True, stop=True)
            mq2 = work.tile([1, MMC], f32, tag="mq2", bufs=1)
            nc.scalar.activation(mq2[:, :w], ps2[:1, :w], AF.Copy,
                                 scale=1.0 / COUT)
            msqc2 = work.tile([1, MMC], f32, tag="msqc2", bufs=1)
            nc.vector.tensor_tensor(out=msqc2[:, :w], in0=mean2[:, s:s + w],
                                    in1=mean2[:, s:s + w], op=OP_.mult)
            nc.vector.tensor_tensor(out=mq2[:, :w], in0=mq2[:, :w],
                                    in1=msqc2[:, :w], op=OP_.subtract)
            nc.scalar.activation(mq2[:, :w], mq2[:, :w], AF.Sqrt, bias=epsc[:])
            with nc.allow_low_precision(reason="bf16 1/std is well conditioned"):
                nc.vector.reciprocal(rs2[:, s:s + w], mq2[:, :w])
        for (s, w) in MM:
            pm = psM.tile([128, MMC], f32, tag="mm", name="psfm")
            nc.tensor.matmul(pm[:, :w], onesrow[:], mean2[:, s:s + w],
                             start=True, stop=True)
            pr = psM.tile([128, MMC], f32, tag="mm", name="psfr")
            nc.tensor.matmul(pr[:, :w], onesrow[:], rs2[:, s:s + w],
                             start=True, stop=True)
            oc = work.tile([128, MMC], f32, tag="oc", bufs=1)
            nc.vector.tensor_tensor(out=oc[:COUT, :w], in0=x3f[:, s:s + w],
                                    in1=pm[:COUT, :w], op=OP_.subtract)
            nc.vector.tensor_tensor(out=oc[:COUT, :w], in0=oc[:COUT, :w],
                                    in1=pr[:COUT, :w], op=OP_.mult)
            nc.vector.tensor_scalar(out=oc[:COUT, :w], in0=oc[:COUT, :w],
                                    scalar1=fwc[:], scalar2=fbc[:],
                                    op0=OP_.mult, op1=OP_.add)
            nc.sync.dma_start(out_d[:, s:s + w], oc[:COUT, :w])
    nc.compile()
    return nc


_NC1, _NC2 = None, None


def _get_ncs():
    global _NC1, _NC2
    if _NC1 is None:
        _NC1 = build_nc1()
        _NC2 = build_nc2()
    return _NC1, _NC2


def _bf(a):
    import jax.numpy as jnp
    return np.asarray(jnp.asarray(np.asarray(a, np.float32), jnp.bfloat16))


def _diag9(wmat, rows):
    out = np.zeros((rows, 9, rows), np.float32)
    idx = np.arange(rows)
    for j in range(9):
        out[idx, j, idx] = wmat[:, j]
    return out


def prep_pass1(ip):
    W1 = (np.diag(ip["ln1_w"]) @ ip["in_proj_W"]).astype(np.float32)
    b1 = (ip["ln1_b"] @ ip["in_proj_W"] + ip["in_proj_b"]).astype(np.float32)
    A = (-np.exp(ip["A_logs"].astype(np.float64))).astype(np.float32).reshape(KDIR, DIN, NST)
    Ds = ip["Ds"].reshape(KDIR, DIN)
    col = lambda v: np.ascontiguousarray(v.reshape(-1, 1), dtype=np.float32)
    convW = ip["conv_W"].reshape(DIN, 9)
    base = dict(projW=ip["proj_W"], projb=col(ip["proj_b"]), W1=_bf(W1),
                b1=col(b1),
                cdiag0=_bf(_diag9(convW[0:DT0], DT0)),
                cdiag1=_bf(_diag9(convW[DT0:DIN], DT1)),
                convb=col(ip["conv_b"]),
                eye=_bf(np.eye(128, dtype=np.float32)))
    maps = []
    for c in range(8):
        b, plane = c // 2, c % 2
        ks = [plane, plane + 2]
        m = dict(base)
        m["xc_t"] = np.ascontiguousarray(ip["x_cat"][b].reshape(L, CIN).T)
        m["xpw"] = _bf(np.stack([ip["x_proj_W"][k].T for k in ks]))
        m["dtw"] = _bf(np.stack([ip["dt_W"][k].T for k in ks]))
        m["dtb"] = np.ascontiguousarray(np.stack([col(ip["dt_b"][k]) for k in ks]))
        m["acoef"] = np.ascontiguousarray(np.stack([A[k] for k in ks]))
        m["dsum"] = col(Ds[ks[0]] + Ds[ks[1]])
        m["mrow"] = np.full((DIN, 1), 1.0 - plane, np.float32)
        m["mcol"] = np.full((DIN, 1), float(plane), np.float32)
        maps.append(m)
    return maps


def prep_pass2(ip, res1):
    OPm = (np.diag(ip["out_norm_w"]) @ ip["out_proj_W"]).astype(np.float32)
    OPB = (np.diag(ip["out_norm_b"]) @ ip["out_proj_W"]).astype(np.float32)
    col = lambda v: np.ascontiguousarray(v.reshape(-1, 1), dtype=np.float32)
    cbw = ip["cb_dw_W"].reshape(HID, 9)
    base = dict(OPm=_bf(OPm), OPB=_bf(OPB),
                PW1=_bf(ip["cb_pw1_W"][:, :, 0, 0].T),
                g1=col(ip["cb_bn1_g"]), bb1=col(ip["cb_bn1_b"]),
                cbdiag0=_bf(_diag9(cbw[0:DT0], DT0)),
                cbdiag1=_bf(_diag9(cbw[DT0:HID], DT1)),
                g2=col(ip["cb_bn2_g"]), bb2=col(ip["cb_bn2_b"]),
                PW2=_bf(ip["cb_pw2_W"][:, :, 0, 0].T),
                g3=col(ip["cb_bn3_g"]), bb3=col(ip["cb_bn3_b"]),
                fw=col(ip["norm_w"]), fb=col(ip["norm_b"]))
    maps = []
    for c in range(8):
        b = c // 2
        m = dict(base)
        ymf = (np.asarray(res1[2 * b]["oq"], np.float32)
               + np.asarray(res1[2 * b + 1]["oq"], np.float32))
        m["ym"] = _bf(ymf)
        m["xin"] = np.asarray(res1[2 * b]["ox"], np.float32)
        m["zin"] = np.ascontiguousarray(res1[2 * b]["oz"])
        maps.append(m)
    return maps


def kernel(**inputs):
    ip = {k: np.asarray(v, np.float32) for k, v in inputs.items()}
    nc1, nc2 = _get_ncs()
    res1 = run_bass_kernel_spmd(nc1, prep_pass1(ip), list(range(8))).results
    res2 = run_bass_kernel_spmd(nc2, prep_pass2(ip, res1), list(range(8))).results
    outs = [np.asarray(res2[2 * b]["o"], np.float32).T.reshape(H_, W_, COUT)
            for b in range(B_)]
    return np.stack(outs).astype(np.float32)


# revision 35
# speedup vs baseline: 1.8404x; 1.0706x over previous
"""Trainium2 Bass kernel for nn_DecoderFusionBlock (VSS/Mamba decoder fusion block).

Two-pass SPMD over 8 cores:
  pass 1: core c -> batch b=c//2, plane=c%2 (row-/col-major spatial order).
          proj/LN/in_proj (f32r / bf16 matmuls), depthwise conv via PE diag
          matmuls, then the selective scan for the plane's two directions.
          bf16 data path with fp32 scan state; B/C broadcast to all channel
          partitions via a DRAM-staged broadcast DMA so the big elementwise
          multiplies run in the DVE 2x (2-byte) mode; the n-state reduction
          runs on the PE as identity-weight matmul accumulation in PSUM.
  host:   ym[b] = Q[2b] + Q[2b+1]  (the only cross-core reduction)
  pass 2: core c -> batch b=c//2: out-norm, gate, out_proj+residual,
          ConvBlock (conv again via PE), final LayerNorm.
"""

import contextlib
import numpy as np

import concourse.bass as bass
import concourse.tile as tile
from concourse import bacc, mybir
from concourse.bass_utils import run_bass_kernel_spmd

f32 = mybir.dt.float32
f32r = mybir.dt.float32r
bf16 = mybir.dt.bfloat16
AF = mybir.ActivationFunctionType
OP_ = mybir.AluOpType

B_, H_, W_ = 4, 48, 48
L = H_ * W_
CIN, COUT = 192, 96
DIN, NST, RNK, KDIR = 192, 16, 6, 4
HID = 192
EPS = 1e-5
DT0, DT1 = 128, 64
MMC = 512
MM = [(s, min(MMC, L - s)) for s in range(0, L, MMC)]
LC = 256
SC = [(i * LC, LC) for i in range(L // LC)]
CROWS = [(0, 10), (10, 10), (20, 10), (30, 10), (40, 8)]


def _rev(ap, s, w):
    hi = L - 1 - s
    lo = hi - w
    return ap[:, hi::-1] if lo < 0 else ap[:, hi:lo:-1]


def _sl(ap, k, s, w):
    return ap[:, s:s + w] if k == 0 else _rev(ap, s, w)


def _rep(a, n):
    return bass.AP(tensor=a.tensor, offset=a.offset, ap=[a.ap[0], [0, n], a.ap[1]])


def _twh(a):
    st = a.ap[1][0]
    return bass.AP(tensor=a.tensor, offset=a.offset,
                   ap=[a.ap[0], [st, 48], [48 * st, 48]])


def _pl3(a):
    st = a.ap[1][0]
    return bass.AP(tensor=a.tensor, offset=a.offset,
                   ap=[a.ap[0], [48 * st, 48], [st, 48]])


# ---------------------------------------------------------------- pass 1
def build_nc1():
    nc = bacc.Bacc("TRN2", target_bir_lowering=False, debug=False, num_devices=8)
    din = {}

    def I(name, shape, dt=f32):
        din[name] = nc.dram_tensor(name, shape, dt, kind="ExternalInput")

    I("xc_t", [CIN, L], f32r)
    I("projW", [CIN, COUT], f32r); I("projb", [COUT, 1])
    I("W1", [COUT, 2 * DIN], bf16); I("b1", [2 * DIN, 1])
    I("cdiag0", [DT0, 9, DT0], bf16); I("cdiag1", [DT1, 9, DT1], bf16)
    I("convb", [DIN, 1])
    I("eye", [128, 128], bf16)
    I("xpw", [2, DIN, RNK + 2 * NST], bf16)
    I("dtw", [2, RNK, DIN], bf16)
    I("dtb", [2, DIN, 1]); I("acoef", [2, DIN, NST]); I("dsum", [DIN, 1])
    I("mrow", [DIN, 1]); I("mcol", [DIN, 1])
    oq_d = nc.dram_tensor("oq", [DIN, L], bf16, kind="ExternalOutput")
    ox_d = nc.dram_tensor("ox", [COUT, L], f32, kind="ExternalOutput")
    oz_d = nc.dram_tensor("oz", [DIN, L], bf16, kind="ExternalOutput")
    bcd = nc.dram_tensor("BCd", [2, 32, L], bf16, kind="Internal")

    ctx = contextlib.ExitStack()
    with tile.TileContext(nc) as tc, ctx:
        const = ctx.enter_context(tc.tile_pool(name="const", bufs=1))
        big = ctx.enter_context(tc.tile_pool(name="big", bufs=1))
        work = ctx.enter_context(tc.tile_pool(name="work", bufs=2))
        scn = ctx.enter_context(tc.tile_pool(name="scn", bufs=1))
        psM = ctx.enter_context(tc.tile_pool(name="psM", bufs=2, space="PSUM"))
        psY = ctx.enter_context(tc.tile_pool(name="psY", bufs=2, space="PSUM"))

        def load2(name, rows, cols, dt=f32):
            t0 = const.tile([DT0, cols], dt, tag=name + "0", name=name + "0")
            t1 = const.tile([DT1, cols], dt, tag=name + "1", name=name + "1")
            nc.sync.dma_start(t0[:], din[name][0:DT0])
            nc.sync.dma_start(t1[:], din[name][DT0:rows])
            return t0, t1

        # input data first so the proj chain isn't stuck behind const loads
        xc0 = big.tile([DT0, L], f32r, tag="xc0")
        xc1 = big.tile([DT1, L], f32r, tag="xc1")
        nc.sync.dma_start(xc0[:], din["xc_t"][0:DT0])
        nc.sync.dma_start(xc1[:], din["xc_t"][DT0:CIN])
        projW0 = const.tile([DT0, COUT], f32r)
        projW1 = const.tile([DT1, COUT], f32r)
        nc.sync.dma_start(projW0[:], din["projW"][0:DT0])
        nc.sync.dma_start(projW1[:], din["projW"][DT0:CIN])
        projb = const.tile([COUT, 1], f32)
        nc.sync.dma_start(projb[:], din["projb"][:])
        W1t = const.tile([COUT, 2 * DIN], bf16)
        nc.sync.dma_start(W1t[:], din["W1"][:])
        b1x0 = const.tile([DT0, 1], f32); nc.sync.dma_start(b1x0[:], din["b1"][0:128])
        b1x1 = const.tile([DT1, 1], f32); nc.sync.dma_start(b1x1[:], din["b1"][128:192])
        b1z0 = const.tile([DT0, 1], f32); nc.sync.dma_start(b1z0[:], din["b1"][192:320])
        b1z1 = const.tile([DT1, 1], f32); nc.sync.dma_start(b1z1[:], din["b1"][320:384])
        cdiag0 = const.tile([DT0, 9, DT0], bf16)
        nc.sync.dma_start(cdiag0[:], din["cdiag0"][:])
        cdiag1 = const.tile([DT1, 9, DT1], bf16)
        nc.sync.dma_start(cdiag1[:], din["cdiag1"][:])
        convb0, convb1 = load2("convb", DIN, 1)
        eye = const.tile([128, 128], bf16)
        nc.sync.dma_start(eye[:], din["eye"][:])
        dsum0, dsum1 = load2("dsum", DIN, 1)
        mrow0, mrow1 = load2("mrow", DIN, 1)
        mcol0, mcol1 = load2("mcol", DIN, 1)
        kw = []
        for k in range(2):
            xp0 = const.tile([DT0, RNK + 2 * NST], bf16, name=f"xp{k}0")
            xp1 = const.tile([DT1, RNK + 2 * NST], bf16, name=f"xp{k}1")
            nc.sync.dma_start(xp0[:], din["xpw"][k, 0:DT0])
            nc.sync.dma_start(xp1[:], din["xpw"][k, DT0:DIN])
            dtw = const.tile([38, DIN], bf16, tag="dtwm", name=f"dtw{k}",
                             bufs=1) if k == 0 else kw[0]["dtwt"]
            nc.sync.dma_start(dtw[k * 32:k * 32 + RNK], din["dtw"][k])
            dtb0 = const.tile([DT0, 1], f32, name=f"dtb{k}0")
            dtb1 = const.tile([DT1, 1], f32, name=f"dtb{k}1")
            nc.sync.dma_start(dtb0[:], din["dtb"][k, 0:DT0])
            nc.sync.dma_start(dtb1[:], din["dtb"][k, DT0:DIN])
            ac0 = const.tile([DT0, NST], f32, name=f"ac{k}0")
            ac1 = const.tile([DT1, NST], f32, name=f"ac{k}1")
            nc.sync.dma_start(ac0[:], din["acoef"][k, 0:DT0])
            nc.sync.dma_start(ac1[:], din["acoef"][k, DT0:DIN])
            kw.append(dict(xp=(xp0, xp1), dtwt=dtw,
                           dtw=dtw[k * 32:k * 32 + RNK], dtb=(dtb0, dtb1),
                           ac=(ac0, ac1)))

        ones128 = const.tile([128, 1], f32); nc.vector.memset(ones128[:], 1.0)
        onesrow = const.tile([1, 128], bf16); nc.vector.memset(onesrow[:], 1.0)
        epsc = const.tile([1, 1], f32); nc.vector.memset(epsc[:], EPS)

        # ---- proj (f32r matmuls, x_t kept fp32 for residual) ----
        x_t = big.tile([COUT, L], f32, tag="x_t")
        for (s, w) in MM:
            ps = psM.tile([128, MMC], f32, tag="mm", name="psproj")
            nc.tensor.matmul(ps[:COUT, :w], projW0[:], xc0[:, s:s + w],
                             start=True, stop=False)
            nc.tensor.matmul(ps[:COUT, :w], projW1[:], xc1[:, s:s + w],
                             start=False, stop=True)
            nc.scalar.activation(x_t[:, s:s + w], ps[:COUT, :w], AF.Identity,
                                 bias=projb[:])
        nc.sync.dma_start(ox_d[:], x_t[:])

        # ---- LN1 (Copy + Sqrt share the act-table phase) -> xn bf16 ----
        xn_t = big.tile([COUT, L], bf16, tag="xn")
        for (s, w) in MM:
            ps1 = psM.tile([128, MMC], f32, tag="mm", name="pss1")
            nc.tensor.matmul(ps1[:1, :w], ones128[:COUT], x_t[:, s:s + w],
                             start=True, stop=True)
            mrw = work.tile([1, MMC], bf16, tag="mrw", bufs=1)
            nc.scalar.activation(mrw[:, :w], ps1[:1, :w], AF.Copy, scale=1.0 / COUT)
            sq = work.tile([128, MMC], f32, tag="sqc", bufs=1)
            nc.vector.tensor_tensor(out=sq[:COUT, :w], in0=x_t[:, s:s + w],
                                    in1=x_t[:, s:s + w], op=OP_.mult)
            ps2 = psM.tile([128, MMC], f32, tag="mm", name="pss2")
            nc.tensor.matmul(ps2[:1, :w], ones128[:COUT], sq[:COUT, :w],
                             start=True, stop=True)
            mq = work.tile([1, MMC], f32, tag="mq", bufs=1)
            nc.scalar.activation(mq[:, :w], ps2[:1, :w], AF.Copy, scale=1.0 / COUT)
            msq = work.tile([1, MMC], f32, tag="msq", bufs=1)
            nc.vector.tensor_tensor(out=msq[:, :w], in0=mrw[:, :w],
                                    in1=mrw[:, :w], op=OP_.mult)
            nc.vector.tensor_tensor(out=mq[:, :w], in0=mq[:, :w],
                                    in1=msq[:, :w], op=OP_.subtract)
            nc.scalar.activation(mq[:, :w], mq[:, :w], AF.Sqrt, bias=epsc[:])
            rsw = work.tile([1, MMC], bf16, tag="rsw", bufs=1)
            with nc.allow_low_precision(reason="bf16 1/std is well conditioned"):
                nc.vector.reciprocal(rsw[:, :w], mq[:, :w])
            pm = psM.tile([128, MMC], f32, tag="mm", name="psbm")
            nc.tensor.matmul(pm[:, :w], onesrow[:], mrw[:, :w],
                             start=True, stop=True)
            pr = psM.tile([128, MMC], f32, tag="mm", name="psbr")
            nc.tensor.matmul(pr[:, :w], onesrow[:], rsw[:, :w],
                             start=True, stop=True)
            xn_ = work.tile([128, MMC], bf16, tag="xn_", bufs=1)
            nc.vector.tensor_tensor(out=xn_[:COUT, :w], in0=x_t[:, s:s + w],
                                    in1=pm[:COUT, :w], op=OP_.subtract)
            nc.vector.tensor_tensor(out=xn_t[:, s:s + w], in0=xn_[:COUT, :w],
                                    in1=pr[:COUT, :w], op=OP_.mult)

        # ---- in_proj (xm tiles bf16; z silu'd -> DRAM bf16) ----
        xm0 = big.tile([DT0, L], bf16, tag="xm0")
        xm1 = big.tile([DT1, L], bf16, tag="xm1")
        for (s, w) in MM:
            for (coff, rows, bcol, dst, zoff) in (
                    (0, DT0, b1x0, xm0, None), (DT0, DT1, b1x1, xm1, None),
                    (DIN, DT0, b1z0, None, 0), (DIN + DT0, DT1, b1z1, None, DT0)):
                psi = psM.tile([128, MMC], f32, tag="mm", name="psip")
                nc.tensor.matmul(psi[:rows, :w], W1t[:, coff:coff + rows],
                                 xn_t[:, s:s + w], start=True, stop=True)
                if dst is not None:
                    nc.scalar.activation(dst[:, s:s + w], psi[:rows, :w],
                                         AF.Identity, bias=bcol[:])
                else:
                    zc = work.tile([128, MMC], bf16, tag="zc", bufs=1)
                    nc.scalar.activation(zc[:rows, :w], psi[:rows, :w], AF.Silu,
                                         bias=bcol[:])
                    nc.sync.dma_start(oz_d[zoff:zoff + rows, s:s + w], zc[:rows, :w])

        # ---- depthwise conv via PE diag matmuls + fused SiLU; the
        #      transposed copy for the plane transform happens per row-chunk
        xs0 = big.tile([DT0, L], bf16, tag="xs0")
        xs1 = big.tile([DT1, L], bf16, tag="xs1")
        xt0 = big.tile([DT0, L], bf16, tag="xm0", name="xt0")
        xt1 = big.tile([DT1, L], bf16, tag="xm1", name="xt1")
        for (src, cd, rows, out, bcol, xtt, teng) in (
                (xm0, cdiag0, DT0, xs0, convb0, xt0, nc.vector),
                (xm1, cdiag1, DT1, xs1, convb1, xt1, nc.vector)):
            pad = work.tile([128, 50, 50], bf16, tag="pad", bufs=1)
            nc.vector.memset(pad[:rows], 0.0)
            nc.vector.tensor_copy(out=pad[:rows, 1:49, 1:49], in_=_pl3(src[:]))
            for (y0, ny) in CROWS:
                pc = psM.tile([128, MMC], f32, tag="mm", name="pscv")
                for j in range(9):
                    dy, dx = divmod(j, 3)
                    view = pad[:rows, y0 + dy:y0 + dy + ny, dx:dx + 48]
                    nc.tensor.matmul(pc[:rows, :ny * 48], cd[:, j], view,
                                     start=(j == 0), stop=(j == 8))
                nc.scalar.activation(out[:, y0 * 48:(y0 + ny) * 48],
                                     pc[:rows, :ny * 48], AF.Silu, bias=bcol[:])
                # xt[p, x, y] = xs[p, y, x] for this y-chunk
                xin = _pl3(out[:])[:, y0:y0 + ny, :]
                xout = bass.AP(tensor=xtt.tensor, offset=xtt[:].offset + y0,
                               ap=[xtt[:].ap[0], [1, ny], [48, 48]])
                teng.tensor_copy(out=xout, in_=xin)
        nc.vector.tensor_scalar_mul(xs0[:], xs0[:], mrow0[:])
        nc.vector.scalar_tensor_tensor(out=xs0[:], in0=xt0[:], scalar=mcol0[:],
                                       in1=xs0[:], op0=OP_.mult, op1=OP_.add)
        nc.vector.tensor_scalar_mul(xs1[:], xs1[:], mrow1[:])
        nc.vector.scalar_tensor_tensor(out=xs1[:], in0=xt1[:], scalar=mcol1[:],
                                       in1=xs1[:], op0=OP_.mult, op1=OP_.add)

        # ---- U96: x_dbl for both k; B/C staged to DRAM bf16 ----
        u96m = big.tile([38, L], bf16, tag="u96r")
        u96rs = (u96m[0:RNK], u96m[32:38])
        for k in range(2):
            W = kw[k]
            rb = k * 32          # rank section base: 0 (k0) / 32 (k1)
            bb = rb + 32
            cb = 64 if k == 0 else 0
            for (s, w) in MM:
                ps = psM.tile([128, MMC], f32, tag="mm", name="psU")
                for (coff, ubase, m) in ((0, rb, RNK), (RNK, bb, NST),
                                         (RNK + NST, cb, NST)):
                    nc.tensor.matmul(ps[ubase:ubase + m, :w],
                                     W["xp"][0][:, coff:coff + m],
                                     xs0[:, s:s + w], start=True, stop=False)
                    nc.tensor.matmul(ps[ubase:ubase + m, :w],
                                     W["xp"][1][:, coff:coff + m],
                                     xs1[:, s:s + w], start=False, stop=True)
                nc.scalar.activation(u96rs[k][:, s:s + w],
                                     ps[rb:rb + RNK, :w], AF.Copy)
                bcsw = work.tile([112, MMC], bf16, tag="bcsw")
                nc.scalar.activation(bcsw[bb:bb + NST, :w], ps[bb:bb + NST, :w],
                                     AF.Copy)
                nc.scalar.activation(bcsw[cb:cb + NST, :w], ps[cb:cb + NST, :w],
                                     AF.Copy)
                nc.sync.dma_start(bcd[k, 0:NST, s:s + w], bcsw[bb:bb + NST, :w])
                nc.sync.dma_start(bcd[k, NST:32, s:s + w], bcsw[cb:cb + NST, :w])

        # ---- delta (Softplus) for both k, both dt ----
        dcs = []
        for k in range(2):
            W = kw[k]
            dc0 = big.tile([DT0, L], bf16, tag="xc0" if k == 0 else "xc1",
                           name=f"dc{k}0")
            dc1 = big.tile([DT1, L], bf16, tag=f"dc{k}1", name=f"dc{k}1")
            for (s, w) in MM:
                for dt, (rows, dct) in enumerate(((DT0, dc0), (DT1, dc1))):
                    ps = psM.tile([128, MMC], f32, tag="mm", name="psdt")
                    nc.tensor.matmul(ps[:rows, :w],
                                     W["dtw"][:, dt * DT0:dt * DT0 + rows],
                                     u96rs[k][:, s:s + w],
                                     start=True, stop=True)
                    nc.scalar.activation(dct[:, s:s + w], ps[:rows, :w],
                                         AF.Exp, bias=W["dtb"][dt][:])
            # in-place Ln: dc = ln(1 + e^u) = softplus(u), full-L per tile
            nc.scalar.activation(dc0[:], dc0[:], AF.Ln, bias=1.0)
            nc.scalar.activation(dc1[:], dc1[:], AF.Ln, bias=1.0)
            # precompute dxc = delta * x for the whole plane (DVE idle here)
            dx0 = big.tile([DT0, L], bf16, tag="x_t" if k == 0 else "xn",
                           name=f"dx{k}0")
            dx1 = big.tile([DT1, L], bf16, tag="xm1" if k == 0 else "u96r",
                           name=f"dx{k}1")
            nc.vector.tensor_tensor(out=dx0[:], in0=dc0[:], in1=xs0[:],
                                    op=OP_.mult)
            nc.vector.tensor_tensor(out=dx1[:], in0=dc1[:], in1=xs1[:],
                                    op=OP_.mult)
            dcs.append((dc0, dc1, dx0, dx1))

        # ---- P init with the direction-independent D term ----
        P0 = big.tile([DT0, L], bf16, tag="xm0", name="P0")
        P1 = big.tile([DT1, L], bf16, tag="xt1m", name="P1")
        nc.vector.tensor_scalar_mul(P0[:], xs0[:], dsum0[:])
        nc.gpsimd.tensor_scalar_mul(P1[:], xs1[:], dsum1[:])

        # ---- selective scan ----
        hp = [[scn.tile([128, NST, 1], bf16, tag=f"hp{k}{dt}", name=f"hp{k}{dt}")
               for dt in range(2)] for k in range(2)]
        for k in range(2):
            for dt in range(2):
                nc.vector.memset(hp[k][dt][:], 0.0)

        pairs = [(c, k) for c in range(len(SC)) for k in range(2)]
        bcr_tiles = {}

        def issue_bcast(i):
            c, k = pairs[i]
            s, w = SC[c]
            s0 = s if k == 0 else L - s - w
            t = scn.tile([128, 32, LC], bf16, tag="bcr", bufs=3,
                         name=f"bcr{i}")
            src = bass.AP(tensor=bcd, offset=k * 32 * L + s0,
                          ap=[[0, 128], [L, 32], [1, w]])
            nc.sync.dma_start(t[:], src)
            bcr_tiles[i] = t

        def tail(rows, dA, Ht, crep, hpt, pdst):
            # after the scan: save carry, G = H*C, PE n-reduction, P +=
            nc.gpsimd.tensor_copy(out=hpt[:rows], in_=Ht[:rows, :, LC:LC + 1])
            nc.vector.tensor_tensor(out=dA[:rows, :, 1:], in0=Ht[:rows, :, 1:],
                                    in1=crep[:rows], op=OP_.mult)
            psy = psY.tile([128, LC], f32, tag="psy", name="psy")
            for n in range(NST):
                nc.tensor.matmul(psy[:rows, :], eye[:rows, :rows],
                                 dA[:rows, n, 1:],
                                 start=(n == 0), stop=(n == NST - 1))
            nc.vector.tensor_tensor(out=pdst, in0=psy[:rows, :], in1=pdst,
                                    op=OP_.add)

        pending = []
        issue_bcast(0)
        for i, (c, k) in enumerate(pairs):
            s, w = SC[c]
            W = kw[k]
            bcr = bcr_tiles.pop(i)
            if k == 0:
                brep = bcr[:, 0:NST, :]
                crep = bcr[:, NST:32, :]
            else:
                brep = bcr[:, 0:NST, ::-1]
                crep = bcr[:, NST:32, ::-1]
            for dt, (rows, xst) in enumerate(((DT0, xs0), (DT1, xs1))):
                if len(pending) >= 3:
                    # flush the 3-iterations-old deferred tail before its
                    # dA/dBu/Ht buffer slots are reused below
                    pending.pop(0)()
                if dt == 0 and i + 1 < len(pairs):
                    # safe point: all readers of bcr slot (i+1)%3's previous
                    # occupant (pair i-2) have been emitted by now
                    issue_bcast(i + 1)
                dct = dcs[k][dt]
                dcsl = _sl(dct[:], k, s, w)
                dxsl = _sl(dcs[k][2 + dt][:], k, s, w)
                dA = scn.tile([128, NST, LC + 1], bf16, tag="dA",
                              name=f"dA{dt}", bufs=3)
                dBu = scn.tile([128, NST, LC + 1], bf16, tag="dBu",
                               name=f"dBu{dt}", bufs=2)
                Ht = scn.tile([128, NST, LC + 1], bf16, tag="Ht",
                              name=f"Ht{dt}", bufs=3)
                nc.gpsimd.memset(dA[:rows, :, 0:1], 0.0)
                nc.gpsimd.tensor_copy(out=dBu[:rows, :, 0:1], in_=hp[k][dt][:rows])
                for n in range(NST):
                    nc.scalar.activation(dA[:rows, n, 1:], dcsl, AF.Exp,
                                         scale=W["ac"][dt][:, n:n + 1])
                beng = nc.vector if dt == 0 else nc.gpsimd
                beng.tensor_tensor(out=dBu[:rows, :, 1:],
                                   in0=_rep(dxsl, NST),
                                   in1=brep[:rows], op=OP_.mult)
                nc.vector.tensor_tensor_scan(
                    out=Ht[:rows].rearrange("p a b -> p (a b)"),
                    data0=dA[:rows].rearrange("p a b -> p (a b)"),
                    data1=dBu[:rows].rearrange("p a b -> p (a b)"),
                    initial=0.0, op0=OP_.mult, op1=OP_.add)
                Pt = P0 if dt == 0 else P1
                args = (rows, dA, Ht, crep, hp[k][dt], _sl(Pt[:], k, s, w))
                pending.append(lambda a=args: tail(*a))
        for fn in pending:
            fn()

        # ---- Q = mrow*P + mcol*transpose(P) ----
        Q0 = big.tile([DT0, L], bf16, tag="xs0", name="Q0")
        Q1 = big.tile([DT1, L], bf16, tag="xs1", name="Q1")
        nc.vector.tensor_scalar_mul(Q0[:], _twh(P0[:]), mcol0[:])
        nc.vector.scalar_tensor_tensor(out=Q0[:], in0=P0[:], scalar=mrow0[:],
                                       in1=Q0[:], op0=OP_.mult, op1=OP_.add)
        nc.gpsimd.tensor_scalar_mul(Q1[:], _twh(P1[:]), mcol1[:])
        qtm = big.tile([DT1, L], bf16, tag="xm1", name="qtm")
        nc.gpsimd.tensor_scalar_mul(qtm[:], P1[:], mrow1[:])
        nc.gpsimd.tensor_tensor(out=Q1[:], in0=qtm[:], in1=Q1[:], op=OP_.add)
        nc.sync.dma_start(oq_d[0:DT0], Q0[:])
        nc.sync.dma_start(oq_d[DT0:DIN], Q1[:])
    nc.compile()
    return nc


# ---------------------------------------------------------------- pass 2
def build_nc2():
    nc = bacc.Bacc("TRN2", target_bir_lowering=False, debug=False, num_devices=8)
    din = {}

    def I(name, shape, dt=f32):
        din[name] = nc.dram_tensor(name, shape, dt, kind="ExternalInput")

    I("ym", [DIN, L], bf16); I("xin", [COUT, L]); I("zin", [DIN, L], bf16)
    I("OPm", [DIN, COUT], bf16); I("OPB", [DIN, COUT], bf16)
    I("PW1", [COUT, HID], bf16); I("g1", [HID, 1]); I("bb1", [HID, 1])
    I("cbdiag0", [DT0, 9, DT0], bf16); I("cbdiag1", [DT1, 9, DT1], bf16)
    I("g2", [HID, 1]); I("bb2", [HID, 1])
    I("PW2", [HID, COUT], bf16); I("g3", [COUT, 1]); I("bb3", [COUT, 1])
    I("fw", [COUT, 1]); I("fb", [COUT, 1])
    out_d = nc.dram_tensor("o", [COUT, L], f32, kind="ExternalOutput")

    ctx = contextlib.ExitStack()
    with tile.TileContext(nc) as tc, ctx:
        const = ctx.enter_context(tc.tile_pool(name="const", bufs=1))
        big = ctx.enter_context(tc.tile_pool(name="big", bufs=1))
        work = ctx.enter_context(tc.tile_pool(name="work", bufs=2))
        psM = ctx.enter_context(tc.tile_pool(name="psM", bufs=2, space="PSUM"))

        def load2(name, rows, cols, dt=f32):
            t0 = const.tile([DT0, cols], dt, tag=name + "0", name=name + "0")
            t1 = const.tile([DT1, cols], dt, tag=name + "1", name=name + "1")
            nc.sync.dma_start(t0[:], din[name][0:DT0])
            nc.sync.dma_start(t1[:], din[name][DT0:rows])
            return t0, t1

        def load1(name, rows):
            t = const.tile([rows, 1], f32, tag=name, name=name)
            nc.sync.dma_start(t[:], din[name][:])
            return t

        # input data first so the out-norm chain isn't stuck behind consts
        ym0 = big.tile([DT0, L], bf16, tag="ym0")
        ym1 = big.tile([DT1, L], bf16, tag="ym1")
        nc.sync.dma_start(ym0[:], din["ym"][0:DT0])
        nc.sync.dma_start(ym1[:], din["ym"][DT0:DIN])
        xres = big.tile([COUT, L], f32, tag="xres")
        nc.sync.dma_start(xres[:], din["xin"][:])
        zc0 = big.tile([DT0, L], bf16, tag="zc0")
        zc1 = big.tile([DT1, L], bf16, tag="zc1")
        nc.sync.dma_start(zc0[:], din["zin"][0:DT0])
        nc.sync.dma_start(zc1[:], din["zin"][DT0:DIN])
        OP0, OP1 = load2("OPm", DIN, COUT, bf16)
        OPB0, OPB1 = load2("OPB", DIN, COUT, bf16)
        PW1t = const.tile([COUT, HID], bf16)
        nc.sync.dma_start(PW1t[:], din["PW1"][:])
        g1c0, g1c1 = load2("g1", HID, 1)
        bb1c0, bb1c1 = load2("bb1", HID, 1)
        cbd0 = const.tile([DT0, 9, DT0], bf16)
        nc.sync.dma_start(cbd0[:], din["cbdiag0"][:])
        cbd1 = const.tile([DT1, 9, DT1], bf16)
        nc.sync.dma_start(cbd1[:], din["cbdiag1"][:])
        g2c0, g2c1 = load2("g2", HID, 1)
        bb2c0, bb2c1 = load2("bb2", HID, 1)
        PW20, PW21 = load2("PW2", HID, COUT, bf16)
        g3c = load1("g3", COUT); bb3c = load1("bb3", COUT)
        fwc = load1("fw", COUT); fbc = load1("fb", COUT)
        onesb = const.tile([128, 1], bf16); nc.vector.memset(onesb[:], 1.0)
        onesrow = const.tile([1, 128], bf16); nc.vector.memset(onesrow[:], 1.0)
        epsc = const.tile([1, 1], f32); nc.vector.memset(epsc[:], EPS)

        # out-norm stats over 192 partitions (per-chunk)
        mean_r = big.tile([1, L], bf16, tag="mean")
        rs_r = big.tile([1, L], bf16, tag="rs")
        for (s, w) in MM:
            ps = psM.tile([128, MMC], f32, tag="mm", name="pso1")
            nc.tensor.matmul(ps[:1, :w], onesb[:], ym0[:, s:s + w],
                             start=True, stop=False)
            nc.tensor.matmul(ps[:1, :w], onesb[:DT1], ym1[:, s:s + w],
                             start=False, stop=True)
            nc.scalar.activation(mean_r[:, s:s + w], ps[:1, :w], AF.Copy,
                                 scale=1.0 / DIN)
            ps2 = psM.tile([128, MMC], f32, tag="mm", name="pso2")
            for i, (t, rows) in enumerate(((ym0, DT0), (ym1, DT1))):
                sq = work.tile([128, MMC], bf16, tag="sqc", bufs=1)
                nc.vector.tensor_tensor(out=sq[:rows, :w], in0=t[:, s:s + w],
                                        in1=t[:, s:s + w], op=OP_.mult)
                nc.tensor.matmul(ps2[:1, :w], onesb[:rows], sq[:rows, :w],
                                 start=(i == 0), stop=(i == 1))
            mq = work.tile([1, MMC], f32, tag="mq", bufs=1)
            nc.scalar.activation(mq[:, :w], ps2[:1, :w], AF.Copy,
                                 scale=1.0 / DIN)
            msqc = work.tile([1, MMC], f32, tag="msqc", bufs=1)
            nc.vector.tensor_tensor(out=msqc[:, :w], in0=mean_r[:, s:s + w],
                                    in1=mean_r[:, s:s + w], op=OP_.mult)
            nc.vector.tensor_tensor(out=mq[:, :w], in0=mq[:, :w],
                                    in1=msqc[:, :w], op=OP_.subtract)
            nc.scalar.activation(mq[:, :w], mq[:, :w], AF.Sqrt, bias=epsc[:])
            with nc.allow_low_precision(reason="bf16 1/std is well conditioned"):
                nc.vector.reciprocal(rs_r[:, s:s + w], mq[:, :w])

        x2f = big.tile([COUT, L], f32, tag="x2f")
        x2b = big.tile([COUT, L], bf16, tag="x2b")
        for (s, w) in MM:
            pm = psM.tile([128, MMC], f32, tag="mm", name="psm")
            nc.tensor.matmul(pm[:, :w], onesrow[:], mean_r[:, s:s + w],
                             start=True, stop=True)
            pr = psM.tile([128, MMC], f32, tag="mm", name="psr")
            nc.tensor.matmul(pr[:, :w], onesrow[:], rs_r[:, s:s + w],
                             start=True, stop=True)
            po = psM.tile([128, MMC], f32, tag="mm", name="pso")
            for i, (t, z, rows) in enumerate(((ym0, zc0, DT0), (ym1, zc1, DT1))):
                yn = work.tile([128, MMC], bf16, tag=f"yn{i}", name=f"yn{i}")
                nc.vector.tensor_tensor(out=yn[:rows, :w], in0=t[:, s:s + w],
                                        in1=pm[:rows, :w], op=OP_.subtract)
                nc.vector.tensor_tensor(out=yn[:rows, :w], in0=yn[:rows, :w],
                                        in1=pr[:rows, :w], op=OP_.mult)
                nc.vector.tensor_tensor(out=yn[:rows, :w], in0=yn[:rows, :w],
                                        in1=z[:, s:s + w], op=OP_.mult)
                OPt = OP0 if i == 0 else OP1
                OPBt = OPB0 if i == 0 else OPB1
                nc.tensor.matmul(po[:COUT, :w], OPt[:], yn[:rows, :w],
                                 start=(i == 0), stop=False)
                nc.tensor.matmul(po[:COUT, :w], OPBt[:], z[:, s:s + w],
                                 start=False, stop=(i == 1))
            nc.vector.tensor_tensor(out=x2f[:, s:s + w], in0=po[:COUT, :w],
                                    in1=xres[:, s:s + w], op=OP_.add)
            nc.scalar.activation(x2b[:, s:s + w], x2f[:, s:s + w], AF.Copy)

        # ConvBlock: PW1 + gelu
        t0 = big.tile([DT0, L], bf16, tag="ym0", name="t0")
        t1 = big.tile([DT1, L], bf16, tag="ym1", name="t1")
        for (s, w) in MM:
            for (dst, coff, rows, gc_, bc_) in ((t0, 0, DT0, g1c0, bb1c0),
                                                (t1, DT0, DT1, g1c1, bb1c1)):
                ps = psM.tile([128, MMC], f32, tag="mm", name="psp1")
                nc.tensor.matmul(ps[:rows, :w], PW1t[:, coff:coff + rows],
                                 x2b[:, s:s + w], start=True, stop=True)
                nc.scalar.activation(dst[:, s:s + w], ps[:rows, :w], AF.Gelu,
                                     bias=bc_[:], scale=gc_[:])
        # dw conv via PE; fused bn2+gelu on psum
        v0 = big.tile([DT0, L], bf16, tag="zc0", name="v0")
        v1 = big.tile([DT1, L], bf16, tag="zc1", name="v1")
        for (src, cd, rows, out, gc_, bc_) in (
                (t0, cbd0, DT0, v0, g2c0, bb2c0),
                (t1, cbd1, DT1, v1, g2c1, bb2c1)):
            pad = work.tile([128, 50, 50], bf16, tag="pad", bufs=1)
            nc.vector.memset(pad[:rows], 0.0)
            nc.vector.tensor_copy(out=pad[:rows, 1:49, 1:49], in_=_pl3(src[:]))
            for (y0, ny) in CROWS:
                pc = psM.tile([128, MMC], f32, tag="mm", name="pscv")
                for j in range(9):
                    dy, dx = divmod(j, 3)
                    view = pad[:rows, y0 + dy:y0 + dy + ny, dx:dx + 48]
                    nc.tensor.matmul(pc[:rows, :ny * 48], cd[:, j], view,
                                     start=(j == 0), stop=(j == 8))
                nc.scalar.activation(out[:, y0 * 48:(y0 + ny) * 48],
                                     pc[:rows, :ny * 48], AF.Gelu,
                                     bias=bc_[:], scale=gc_[:])
        # PW2 + bn3 + residual
        x3f = big.tile([COUT, L], f32, tag="x3f")
        x3b = big.tile([COUT, L], bf16, tag="xres", name="x3b")
        for (s, w) in MM:
            ps = psM.tile([128, MMC], f32, tag="mm", name="psp2")
            nc.tensor.matmul(ps[:COUT, :w], PW20[:], v0[:, s:s + w],
                             start=True, stop=False)
            nc.tensor.matmul(ps[:COUT, :w], PW21[:], v1[:, s:s + w],
                             start=False, stop=True)
            cbt = work.tile([128, MMC], bf16, tag="cbt", bufs=1)
            nc.scalar.activation(cbt[:COUT, :w], ps[:COUT, :w], AF.Identity,
                                 bias=bb3c[:], scale=g3c[:])
            nc.vector.tensor_tensor(out=x3f[:, s:s + w], in0=cbt[:COUT, :w],
                                    in1=x2f[:, s:s + w], op=OP_.add)
            nc.scalar.activation(x3b[:, s:s + w], x3f[:, s:s + w], AF.Copy)

        # final LN
        mean2 = big.tile([1, L], bf16, tag="mean2")
        rs2 = big.tile([1, L], bf16, tag="rs2")
        for (s, w) in MM:
            ps = psM.tile([128, MMC], f32, tag="mm", name="psf1")
            nc.tensor.matmul(ps[:1, :w], onesb[:COUT], x3b[:, s:s + w],
                             start=True, stop=True)
            nc.scalar.activation(mean2[:, s:s + w], ps[:1, :w], AF.Copy,
                                 scale=1.0 / COUT)
            sq = work.tile([128, MMC], bf16, tag="sqc", bufs=1)
            nc.vector.tensor_tensor(out=sq[:COUT, :w], in0=x3b[:, s:s + w],
                                    in1=x3b[:, s:s + w], op=OP_.mult)
            ps2 = psM.tile([128, MMC], f32, tag="mm", name="psf2")
            nc.tensor.matmul(ps2[:1, :w], onesb[:COUT], sq[:COUT, :w],
                             start=True, stop=True)
            mq2 = work.tile([1, MMC], f32, tag="mq2", bufs=1)
            nc.scalar.activation(mq2[:, :w], ps2[:1, :w], AF.Copy,
                                 scale=1.0 / COUT)
            msqc2 = work.tile([1, MMC], f32, tag="msqc2", bufs=1)
            nc.vector.tensor_tensor(out=msqc2[:, :w], in0=mean2[:, s:s + w],
                                    in1=mean2[:, s:s + w], op=OP_.mult)
            nc.vector.tensor_tensor(out=mq2[:, :w], in0=mq2[:, :w],
                                    in1=msqc2[:, :w], op=OP_.subtract)
            nc.scalar.activation(mq2[:, :w], mq2[:, :w], AF.Sqrt, bias=epsc[:])
            with nc.allow_low_precision(reason="bf16 1/std is well conditioned"):
                nc.vector.reciprocal(rs2[:, s:s + w], mq2[:, :w])
        for (s, w) in MM:
            pm = psM.tile([128, MMC], f32, tag="mm", name="psfm")
            nc.tensor.matmul(pm[:, :w], onesrow[:], mean2[:, s:s + w],
                             start=True, stop=True)
            pr = psM.tile([128, MMC], f32, tag="mm", name="psfr")
            nc.tensor.matmul(pr[:, :w], onesrow[:], rs2[:, s:s + w],
                             start=True, stop=True)
            oc = work.tile([128, MMC], f32, tag="oc", bufs=1)
            nc.vector.tensor_tensor(out=oc[:COUT, :w], in0=x3f[:, s:s + w],
                                    in1=pm[:COUT, :w], op=OP_.subtract)
            nc.vector.tensor_tensor(out=oc[:COUT, :w], in0=oc[:COUT, :w],
                                    in1=pr[:COUT, :w], op=OP_.mult)
            nc.vector.tensor_scalar(out=oc[:COUT, :w], in0=oc[:COUT, :w],
                                    scalar1=fwc[:], scalar2=fbc[:],
                                    op0=OP_.mult, op1=OP_.add)
            nc.sync.dma_start(out_d[:, s:s + w], oc[:COUT, :w])
    nc.compile()
    return nc


_NC1, _NC2 = None, None


def _get_ncs():
    global _NC1, _NC2
    if _NC1 is None:
        _NC1 = build_nc1()
        _NC2 = build_nc2()
    return _NC1, _NC2


def _bf(a):
    import jax.numpy as jnp
    return np.asarray(jnp.asarray(np.asarray(a, np.float32), jnp.bfloat16))


def _diag9(wmat, rows):
    out = np.zeros((rows, 9, rows), np.float32)
    idx = np.arange(rows)
    for j in range(9):
        out[idx, j, idx] = wmat[:, j]
    return out


def prep_pass1(ip):
    W1 = (np.diag(ip["ln1_w"]) @ ip["in_proj_W"]).astype(np.float32)
    b1 = (ip["ln1_b"] @ ip["in_proj_W"] + ip["in_proj_b"]).astype(np.float32)
    A = (-np.exp(ip["A_logs"].astype(np.float64))).astype(np.float32).reshape(KDIR, DIN, NST)
    Ds = ip["Ds"].reshape(KDIR, DIN)
    col = lambda v: np.ascontiguousarray(v.reshape(-1, 1), dtype=np.float32)
    convW = ip["conv_W"].reshape(DIN, 9)
    base = dict(projW=ip["proj_W"], projb=col(ip["proj_b"]), W1=_bf(W1),
                b1=col(b1),
                cdiag0=_bf(_diag9(convW[0:DT0], DT0)),
                cdiag1=_bf(_diag9(convW[DT0:DIN], DT1)),
                convb=col(ip["conv_b"]),
                eye=_bf(np.eye(128, dtype=np.float32)))
    maps = []
    for c in range(8):
        b, plane = c // 2, c % 2
        ks = [plane, plane + 2]
        m = dict(base)
        m["xc_t"] = np.ascontiguousarray(ip["x_cat"][b].reshape(L, CIN).T)
        m["xpw"] = _bf(np.stack([ip["x_proj_W"][k].T for k in ks]))
        m["dtw"] = _bf(np.stack([ip["dt_W"][k].T for k in ks]))
        m["dtb"] = np.ascontiguousarray(np.stack([col(ip["dt_b"][k]) for k in ks]))
        m["acoef"] = np.ascontiguousarray(np.stack([A[k] for k in ks]))
        m["dsum"] = col(Ds[ks[0]] + Ds[ks[1]])
        m["mrow"] = np.full((DIN, 1), 1.0 - plane, np.float32)
        m["mcol"] = np.full((DIN, 1), float(plane), np.float32)
        maps.append(m)
    return maps


def prep_pass2(ip, res1):
    OPm = (np.diag(ip["out_norm_w"]) @ ip["out_proj_W"]).astype(np.float32)
    OPB = (np.diag(ip["out_norm_b"]) @ ip["out_proj_W"]).astype(np.float32)
    col = lambda v: np.ascontiguousarray(v.reshape(-1, 1), dtype=np.float32)
    cbw = ip["cb_dw_W"].reshape(HID, 9)
    base = dict(OPm=_bf(OPm), OPB=_bf(OPB),
                PW1=_bf(ip["cb_pw1_W"][:, :, 0, 0].T),
                g1=col(ip["cb_bn1_g"]), bb1=col(ip["cb_bn1_b"]),
                cbdiag0=_bf(_diag9(cbw[0:DT0], DT0)),
                cbdiag1=_bf(_diag9(cbw[DT0:HID], DT1)),
                g2=col(ip["cb_bn2_g"]), bb2=col(ip["cb_bn2_b"]),
                PW2=_bf(ip["cb_pw2_W"][:, :, 0, 0].T),
                g3=col(ip["cb_bn3_g"]), bb3=col(ip["cb_bn3_b"]),
                fw=col(ip["norm_w"]), fb=col(ip["norm_b"]))
    maps = []
    for c in range(8):
        b = c // 2
        m = dict(base)
        ymf = (np.asarray(res1[2 * b]["oq"], np.float32)
               + np.asarray(res1[2 * b + 1]["oq"], np.float32))
        m["ym"] = _bf(ymf)
        m["xin"] = np.asarray(res1[2 * b]["ox"], np.float32)
        m["zin"] = np.ascontiguousarray(res1[2 * b]["oz"])
        maps.append(m)
    return maps


def kernel(**inputs):
    ip = {k: np.asarray(v, np.float32) for k, v in inputs.items()}
    nc1, nc2 = _get_ncs()
    res1 = run_bass_kernel_spmd(nc1, prep_pass1(ip), list(range(8))).results
    res2 = run_bass_kernel_spmd(nc2, prep_pass2(ip, res1), list(range(8))).results
    outs = [np.asarray(res2[2 * b]["o"], np.float32).T.reshape(H_, W_, COUT)
            for b in range(B_)]
    return np.stack(outs).astype(np.float32)


# revision 38
# speedup vs baseline: 1.9192x; 1.0428x over previous
"""Trainium2 Bass kernel for nn_DecoderFusionBlock (VSS/Mamba decoder fusion block).

Two-pass SPMD over 8 cores:
  pass 1: core c -> batch b=c//2, plane=c%2 (row-/col-major spatial order).
          proj/LN/in_proj (f32r / bf16 matmuls), depthwise conv via PE diag
          matmuls, then the selective scan for the plane's two directions.
          bf16 data path with fp32 scan state; B/C broadcast to all channel
          partitions via a DRAM-staged broadcast DMA so the big elementwise
          multiplies run in the DVE 2x (2-byte) mode; the n-state reduction
          runs on the PE as identity-weight matmul accumulation in PSUM.
  host:   ym[b] = Q[2b] + Q[2b+1]  (the only cross-core reduction)
  pass 2: core c -> batch b=c//2: out-norm, gate, out_proj+residual,
          ConvBlock (conv again via PE), final LayerNorm.
"""

import contextlib
import numpy as np

import concourse.bass as bass
import concourse.tile as tile
from concourse import bacc, mybir
from concourse.bass_utils import run_bass_kernel_spmd

f32 = mybir.dt.float32
f32r = mybir.dt.float32r
bf16 = mybir.dt.bfloat16
AF = mybir.ActivationFunctionType
OP_ = mybir.AluOpType

B_, H_, W_ = 4, 48, 48
L = H_ * W_
CIN, COUT = 192, 96
DIN, NST, RNK, KDIR = 192, 16, 6, 4
HID = 192
EPS = 1e-5
DT0, DT1 = 128, 64
MMC = 512
MM = [(s, min(MMC, L - s)) for s in range(0, L, MMC)]
LC = 256
SC = [(i * LC, LC) for i in range(L // LC)]
CROWS = [(0, 10), (10, 10), (20, 10), (30, 10), (40, 8)]


def _rev(ap, s, w):
    hi = L - 1 - s
    lo = hi - w
    return ap[:, hi::-1] if lo < 0 else ap[:, hi:lo:-1]


def _sl(ap, k, s, w):
    return ap[:, s:s + w] if k == 0 else _rev(ap, s, w)


def _rep(a, n):
    return bass.AP(tensor=a.tensor, offset=a.offset, ap=[a.ap[0], [0, n], a.ap[1]])


def _twh(a):
    st = a.ap[1][0]
    return bass.AP(tensor=a.tensor, offset=a.offset,
                   ap=[a.ap[0], [st, 48], [48 * st, 48]])


def _pl3(a):
    st = a.ap[1][0]
    return bass.AP(tensor=a.tensor, offset=a.offset,
                   ap=[a.ap[0], [48 * st, 48], [st, 48]])


# ---------------------------------------------------------------- pass 1
def build_nc1():
    nc = bacc.Bacc("TRN2", target_bir_lowering=False, debug=False, num_devices=8)
    din = {}

    def I(name, shape, dt=f32):
        din[name] = nc.dram_tensor(name, shape, dt, kind="ExternalInput")

    I("xc_t", [CIN, L], f32r)
    I("projW", [CIN, COUT], f32r); I("projb", [COUT, 1])
    I("W1", [COUT, 2 * DIN], bf16); I("b1", [2 * DIN, 1])
    I("cdiag0", [DT0, 9, DT0], bf16); I("cdiag1", [DT1, 9, DT1], bf16)
    I("convb", [DIN, 1])
    I("eye", [128, 128], bf16)
    I("xpw", [2, DIN, RNK + 2 * NST], bf16)
    I("xpz", [2, 128, RNK + 2 * NST], bf16)
    I("dtw", [2, RNK, DIN], bf16); I("dtwd", [2, RNK, 128], bf16)
    I("dtb", [2, DIN, 1]); I("dtbd", [2, 128, 1])
    I("acoef", [2, DIN, NST]); I("acp", [2, 128, 8]); I("dsum", [DIN, 1])
    I("cdiag1d", [DT1, 9, 128], bf16); I("cbd", [128, 1])
    I("red1", [128, DT1], bf16)
    I("mrow", [DIN, 1]); I("mcol", [DIN, 1])
    oq_d = nc.dram_tensor("oq", [DIN, L], bf16, kind="ExternalOutput")
    ox_d = nc.dram_tensor("ox", [COUT, L], f32, kind="ExternalOutput")
    oz_d = nc.dram_tensor("oz", [DIN, L], bf16, kind="ExternalOutput")
    bcd = nc.dram_tensor("BCd", [2, 32, L], bf16, kind="Internal")

    ctx = contextlib.ExitStack()
    with tile.TileContext(nc) as tc, ctx:
        const = ctx.enter_context(tc.tile_pool(name="const", bufs=1))
        big = ctx.enter_context(tc.tile_pool(name="big", bufs=1))
        work = ctx.enter_context(tc.tile_pool(name="work", bufs=2))
        scn = ctx.enter_context(tc.tile_pool(name="scn", bufs=1))
        psM = ctx.enter_context(tc.tile_pool(name="psM", bufs=2, space="PSUM"))
        psY = ctx.enter_context(tc.tile_pool(name="psY", bufs=2, space="PSUM"))

        def load2(name, rows, cols, dt=f32):
            t0 = const.tile([DT0, cols], dt, tag=name + "0", name=name + "0")
            t1 = const.tile([DT1, cols], dt, tag=name + "1", name=name + "1")
            nc.sync.dma_start(t0[:], din[name][0:DT0])
            nc.sync.dma_start(t1[:], din[name][DT0:rows])
            return t0, t1

        # input data first so the proj chain isn't stuck behind const loads
        xc0 = big.tile([DT0, L], f32r, tag="xc0")
        xc1 = big.tile([DT1, L], f32r, tag="xc1")
        nc.sync.dma_start(xc0[:], din["xc_t"][0:DT0])
        nc.sync.dma_start(xc1[:], din["xc_t"][DT0:CIN])
        projW0 = const.tile([DT0, COUT], f32r)
        projW1 = const.tile([DT1, COUT], f32r)
        nc.sync.dma_start(projW0[:], din["projW"][0:DT0])
        nc.sync.dma_start(projW1[:], din["projW"][DT0:CIN])
        projb = const.tile([COUT, 1], f32)
        nc.sync.dma_start(projb[:], din["projb"][:])
        W1t = const.tile([COUT, 2 * DIN], bf16)
        nc.sync.dma_start(W1t[:], din["W1"][:])
        b1x0 = const.tile([DT0, 1], f32); nc.sync.dma_start(b1x0[:], din["b1"][0:128])
        b1x1 = const.tile([DT1, 1], f32); nc.sync.dma_start(b1x1[:], din["b1"][128:192])
        b1z0 = const.tile([DT0, 1], f32); nc.sync.dma_start(b1z0[:], din["b1"][192:320])
        b1z1 = const.tile([DT1, 1], f32); nc.sync.dma_start(b1z1[:], din["b1"][320:384])
        cdiag0 = const.tile([DT0, 9, DT0], bf16)
        nc.sync.dma_start(cdiag0[:], din["cdiag0"][:])
        cdiag1 = const.tile([DT1, 9, DT1], bf16)
        nc.sync.dma_start(cdiag1[:], din["cdiag1"][:])
        convb0, convb1 = load2("convb", DIN, 1)
        cdiag1d = const.tile([DT1, 9, 128], bf16)
        nc.sync.dma_start(cdiag1d[:], din["cdiag1d"][:])
        cbd = const.tile([128, 1], f32)
        nc.sync.dma_start(cbd[:], din["cbd"][:])
        red1 = const.tile([128, DT1], bf16)
        nc.sync.dma_start(red1[:], din["red1"][:])
        eye = const.tile([128, 128], bf16)
        nc.sync.dma_start(eye[:], din["eye"][:])
        dsum0, dsum1 = load2("dsum", DIN, 1)
        mrow0, mrow1 = load2("mrow", DIN, 1)
        mcol0, mcol1 = load2("mcol", DIN, 1)
        kw = []
        for k in range(2):
            xp0 = const.tile([DT0, RNK + 2 * NST], bf16, name=f"xp{k}0")
            xp1 = const.tile([128, RNK + 2 * NST], bf16, name=f"xp{k}1")
            nc.sync.dma_start(xp0[:], din["xpw"][k, 0:DT0])
            nc.sync.dma_start(xp1[:], din["xpz"][k])
            dtw = const.tile([38, DIN], bf16, tag="dtwm", name=f"dtw{k}",
                             bufs=1) if k == 0 else kw[0]["dtwt"]
            nc.sync.dma_start(dtw[k * 32:k * 32 + RNK], din["dtw"][k])
            dtwd = const.tile([38, 128], bf16, tag="dtwdm", name=f"dtwd{k}",
                              bufs=1) if k == 0 else kw[0]["dtwdt"]
            nc.sync.dma_start(dtwd[k * 32:k * 32 + RNK], din["dtwd"][k])
            dtb0 = const.tile([DT0, 1], f32, name=f"dtb{k}0")
            dtb1 = const.tile([128, 1], f32, name=f"dtb{k}1")
            nc.sync.dma_start(dtb0[:], din["dtb"][k, 0:DT0])
            nc.sync.dma_start(dtb1[:], din["dtbd"][k])
            ac0 = const.tile([DT0, NST], f32, name=f"ac{k}0")
            ac1 = const.tile([128, 8], f32, name=f"ac{k}1")
            nc.sync.dma_start(ac0[:], din["acoef"][k, 0:DT0])
            nc.sync.dma_start(ac1[:], din["acp"][k])
            kw.append(dict(xp=(xp0, xp1), dtwt=dtw, dtwdt=dtwd,
                           dtw=dtw[k * 32:k * 32 + RNK],
                           dtwd=dtwd[k * 32:k * 32 + RNK],
                           dtb=(dtb0, dtb1),
                           ac=(ac0, ac1)))

        ones128 = const.tile([128, 1], f32); nc.vector.memset(ones128[:], 1.0)
        onesrow = const.tile([1, 128], bf16); nc.vector.memset(onesrow[:], 1.0)
        epsc = const.tile([1, 1], f32); nc.vector.memset(epsc[:], EPS)

        # ---- proj (f32r matmuls, x_t kept fp32 for residual) ----
        x_t = big.tile([COUT, L], f32, tag="x_t")
        for (s, w) in MM:
            ps = psM.tile([128, MMC], f32, tag="mm", name="psproj")
            nc.tensor.matmul(ps[:COUT, :w], projW0[:], xc0[:, s:s + w],
                             start=True, stop=False)
            nc.tensor.matmul(ps[:COUT, :w], projW1[:], xc1[:, s:s + w],
                             start=False, stop=True)
            nc.scalar.activation(x_t[:, s:s + w], ps[:COUT, :w], AF.Identity,
                                 bias=projb[:])
        nc.sync.dma_start(ox_d[:], x_t[:])

        # ---- LN1 (Copy + Sqrt share the act-table phase) -> xn bf16 ----
        xn_t = big.tile([COUT, L], bf16, tag="xn")
        for (s, w) in MM:
            ps1 = psM.tile([128, MMC], f32, tag="mm", name="pss1")
            nc.tensor.matmul(ps1[:1, :w], ones128[:COUT], x_t[:, s:s + w],
                             start=True, stop=True)
            mrw = work.tile([1, MMC], bf16, tag="mrw", bufs=1)
            nc.scalar.activation(mrw[:, :w], ps1[:1, :w], AF.Copy, scale=1.0 / COUT)
            sq = work.tile([128, MMC], f32, tag="sqc", bufs=1)
            nc.vector.tensor_tensor(out=sq[:COUT, :w], in0=x_t[:, s:s + w],
                                    in1=x_t[:, s:s + w], op=OP_.mult)
            ps2 = psM.tile([128, MMC], f32, tag="mm", name="pss2")
            nc.tensor.matmul(ps2[:1, :w], ones128[:COUT], sq[:COUT, :w],
                             start=True, stop=True)
            mq = work.tile([1, MMC], f32, tag="mq", bufs=1)
            nc.scalar.activation(mq[:, :w], ps2[:1, :w], AF.Copy, scale=1.0 / COUT)
            msq = work.tile([1, MMC], f32, tag="msq", bufs=1)
            nc.vector.tensor_tensor(out=msq[:, :w], in0=mrw[:, :w],
                                    in1=mrw[:, :w], op=OP_.mult)
            nc.vector.tensor_tensor(out=mq[:, :w], in0=mq[:, :w],
                                    in1=msq[:, :w], op=OP_.subtract)
            nc.scalar.activation(mq[:, :w], mq[:, :w], AF.Sqrt, bias=epsc[:])
            rsw = work.tile([1, MMC], bf16, tag="rsw", bufs=1)
            with nc.allow_low_precision(reason="bf16 1/std is well conditioned"):
                nc.vector.reciprocal(rsw[:, :w], mq[:, :w])
            pm = psM.tile([128, MMC], f32, tag="mm", name="psbm")
            nc.tensor.matmul(pm[:, :w], onesrow[:], mrw[:, :w],
                             start=True, stop=True)
            pr = psM.tile([128, MMC], f32, tag="mm", name="psbr")
            nc.tensor.matmul(pr[:, :w], onesrow[:], rsw[:, :w],
                             start=True, stop=True)
            xn_ = work.tile([128, MMC], bf16, tag="xn_", bufs=1)
            nc.vector.tensor_tensor(out=xn_[:COUT, :w], in0=x_t[:, s:s + w],
                                    in1=pm[:COUT, :w], op=OP_.subtract)
            nc.vector.tensor_tensor(out=xn_t[:, s:s + w], in0=xn_[:COUT, :w],
                                    in1=pr[:COUT, :w], op=OP_.mult)

        # ---- in_proj (xm tiles bf16; z silu'd -> DRAM bf16) ----
        xm0 = big.tile([DT0, L], bf16, tag="xm0")
        xm1 = big.tile([DT1, L], bf16, tag="xm1")
        for (s, w) in MM:
            for (coff, rows, bcol, dst, zoff) in (
                    (0, DT0, b1x0, xm0, None), (DT0, DT1, b1x1, xm1, None),
                    (DIN, DT0, b1z0, None, 0), (DIN + DT0, DT1, b1z1, None, DT0)):
                psi = psM.tile([128, MMC], f32, tag="mm", name="psip")
                nc.tensor.matmul(psi[:rows, :w], W1t[:, coff:coff + rows],
                                 xn_t[:, s:s + w], start=True, stop=True)
                if dst is not None:
                    nc.scalar.activation(dst[:, s:s + w], psi[:rows, :w],
                                         AF.Identity, bias=bcol[:])
                else:
                    zc = work.tile([128, MMC], bf16, tag="zc", bufs=1)
                    nc.scalar.activation(zc[:rows, :w], psi[:rows, :w], AF.Silu,
                                         bias=bcol[:])
                    nc.sync.dma_start(oz_d[zoff:zoff + rows, s:s + w], zc[:rows, :w])

        # ---- depthwise conv via PE diag matmuls + fused SiLU; the
        #      transposed copy for the plane transform happens per row-chunk
        xs0 = big.tile([DT0, L], bf16, tag="xs0")
        xs1 = big.tile([128, L], bf16, tag="xs1")
        xt0 = big.tile([DT0, L], bf16, tag="xm0", name="xt0")
        xt1 = big.tile([128, L], bf16, tag="xm1", name="xt1")
        for (src, cd, srows, orows, out, bcol, xtt, teng) in (
                (xm0, cdiag0, DT0, DT0, xs0, convb0, xt0, nc.vector),
                (xm1, cdiag1d, DT1, 128, xs1, cbd, xt1, nc.vector)):
            pad = work.tile([128, 50, 50], bf16, tag="pad", bufs=1)
            nc.vector.memset(pad[:srows], 0.0)
            nc.vector.tensor_copy(out=pad[:srows, 1:49, 1:49], in_=_pl3(src[:]))
            for (y0, ny) in CROWS:
                pc = psM.tile([128, MMC], f32, tag="mm", name="pscv")
                for j in range(9):
                    dy, dx = divmod(j, 3)
                    view = pad[:srows, y0 + dy:y0 + dy + ny, dx:dx + 48]
                    nc.tensor.matmul(pc[:orows, :ny * 48], cd[:, j], view,
                                     start=(j == 0), stop=(j == 8))
                nc.scalar.activation(out[:, y0 * 48:(y0 + ny) * 48],
                                     pc[:orows, :ny * 48], AF.Silu, bias=bcol[:])
                # xt[p, x, y] = xs[p, y, x] for this y-chunk
                xin = _pl3(out[:])[:, y0:y0 + ny, :]
                xout = bass.AP(tensor=xtt.tensor, offset=xtt[:].offset + y0,
                               ap=[xtt[:].ap[0], [1, ny], [48, 48]])
                teng.tensor_copy(out=xout, in_=xin)
        nc.vector.tensor_scalar_mul(xs0[:], xs0[:], mrow0[:])
        nc.vector.scalar_tensor_tensor(out=xs0[:], in0=xt0[:], scalar=mcol0[:],
                                       in1=xs0[:], op0=OP_.mult, op1=OP_.add)
        nc.vector.tensor_scalar_mul(xs1[:], xs1[:], mrow0[:])
        nc.vector.scalar_tensor_tensor(out=xs1[:], in0=xt1[:], scalar=mcol0[:],
                                       in1=xs1[:], op0=OP_.mult, op1=OP_.add)

        # ---- U96: x_dbl for both k; B/C staged to DRAM bf16 ----
        u96m = big.tile([38, L], bf16, tag="u96r")
        u96rs = (u96m[0:RNK], u96m[32:38])
        for k in range(2):
            W = kw[k]
            rb = k * 32          # rank section base: 0 (k0) / 32 (k1)
            bb = rb + 32
            cb = 64 if k == 0 else 0
            for (s, w) in MM:
                ps = psM.tile([128, MMC], f32, tag="mm", name="psU")
                for (coff, ubase, m) in ((0, rb, RNK), (RNK, bb, NST),
                                         (RNK + NST, cb, NST)):
                    nc.tensor.matmul(ps[ubase:ubase + m, :w],
                                     W["xp"][0][:, coff:coff + m],
                                     xs0[:, s:s + w], start=True, stop=False)
                    nc.tensor.matmul(ps[ubase:ubase + m, :w],
                                     W["xp"][1][:, coff:coff + m],
                                     xs1[:, s:s + w], start=False, stop=True)
                nc.scalar.activation(u96rs[k][:, s:s + w],
                                     ps[rb:rb + RNK, :w], AF.Copy)
                bcsw = work.tile([112, MMC], bf16, tag="bcsw")
                nc.scalar.activation(bcsw[bb:bb + NST, :w], ps[bb:bb + NST, :w],
                                     AF.Copy)
                nc.scalar.activation(bcsw[cb:cb + NST, :w], ps[cb:cb + NST, :w],
                                     AF.Copy)
                nc.sync.dma_start(bcd[k, 0:NST, s:s + w], bcsw[bb:bb + NST, :w])
                nc.sync.dma_start(bcd[k, NST:32, s:s + w], bcsw[cb:cb + NST, :w])

        # ---- delta (Softplus) for both k, both dt ----
        dcs = []
        for k in range(2):
            W = kw[k]
            dc0 = big.tile([DT0, L], bf16, tag="xc0" if k == 0 else "xc1",
                           name=f"dc{k}0")
            dc1 = big.tile([128, L], bf16, tag=f"dc{k}1", name=f"dc{k}1")
            for (s, w) in MM:
                for dt, (rows, stat, dct) in enumerate(
                        ((DT0, W["dtw"][:, 0:DT0], dc0),
                         (128, W["dtwd"], dc1))):
                    ps = psM.tile([128, MMC], f32, tag="mm", name="psdt")
                    nc.tensor.matmul(ps[:rows, :w], stat,
                                     u96rs[k][:, s:s + w],
                                     start=True, stop=True)
                    nc.scalar.activation(dct[:, s:s + w], ps[:rows, :w],
                                         AF.Exp, bias=W["dtb"][dt][:])
            # in-place Ln: dc = ln(1 + e^u) = softplus(u), full-L per tile
            nc.scalar.activation(dc0[:], dc0[:], AF.Ln, bias=1.0)
            nc.scalar.activation(dc1[:], dc1[:], AF.Ln, bias=1.0)
            # precompute dxc = delta * x for the whole plane (DVE idle here)
            dx0 = big.tile([DT0, L], bf16, tag="x_t" if k == 0 else "xn",
                           name=f"dx{k}0")
            dx1 = big.tile([128, L], bf16, tag="xm1" if k == 0 else "u96r",
                           name=f"dx{k}1")
            nc.vector.tensor_tensor(out=dx0[:], in0=dc0[:], in1=xs0[:],
                                    op=OP_.mult)
            nc.vector.tensor_tensor(out=dx1[:], in0=dc1[:], in1=xs1[:],
                                    op=OP_.mult)
            dcs.append((dc0, dc1, dx0, dx1))

        # ---- P init with the direction-independent D term ----
        P0 = big.tile([DT0, L], bf16, tag="xm0", name="P0")
        P1 = big.tile([DT1, L], bf16, tag="xt1m", name="P1")
        nc.vector.tensor_scalar_mul(P0[:], xs0[:], dsum0[:])
        nc.gpsimd.tensor_scalar_mul(P1[:], xs1[0:DT1], dsum1[:])

        # ---- selective scan ----
        hp = [[scn.tile([128, NST if dt == 0 else 8, 1], bf16,
                        tag=f"hp{k}{dt}", name=f"hp{k}{dt}")
               for dt in range(2)] for k in range(2)]
        for k in range(2):
            for dt in range(2):
                nc.vector.memset(hp[k][dt][:], 0.0)

        pairs = [(c, k) for c in range(len(SC)) for k in range(2)]
        bcr_tiles = {}

        def issue_bcast(i):
            c, k = pairs[i]
            s, w = SC[c]
            s0 = s if k == 0 else L - s - w
            t = scn.tile([128, 32, LC], bf16, tag="bcr", bufs=3,
                         name=f"bcr{i}")
            src = bass.AP(tensor=bcd, offset=k * 32 * L + s0,
                          ap=[[0, 128], [L, 32], [1, w]])
            nc.sync.dma_start(t[:], src)
            bcr_tiles[i] = t

        def tail(dt, dA, Ht, crep, hpt, pdst):
            # after the scan: save carry, G = H*C, PE n-reduction, P +=
            nc.gpsimd.tensor_copy(out=hpt[:], in_=Ht[:, :, LC:LC + 1])
            psy = psY.tile([128, LC], f32, tag="psy", name="psy")
            if dt == 0:
                nc.vector.tensor_tensor(out=dA[:, :, 1:], in0=Ht[:, :, 1:],
                                        in1=crep, op=OP_.mult)
                for n in range(NST):
                    nc.tensor.matmul(psy[:, :], eye[:, :],
                                     dA[:, n, 1:],
                                     start=(n == 0), stop=(n == NST - 1))
                nc.vector.tensor_tensor(out=pdst, in0=psy[:DT0, :], in1=pdst,
                                        op=OP_.add)
            else:
                nc.vector.tensor_tensor(out=dA[0:DT1, :, 1:],
                                        in0=Ht[0:DT1, :, 1:],
                                        in1=crep[0:DT1, 0:8], op=OP_.mult)
                nc.vector.tensor_tensor(out=dA[DT1:128, :, 1:],
                                        in0=Ht[DT1:128, :, 1:],
                                        in1=crep[DT1:128, 8:NST], op=OP_.mult)
                for n in range(8):
                    nc.tensor.matmul(psy[:DT1, :], red1[:, :],
                                     dA[:, n, 1:],
                                     start=(n == 0), stop=(n == 7))
                nc.vector.tensor_tensor(out=pdst, in0=psy[:DT1, :], in1=pdst,
                                        op=OP_.add)

        pending = []
        issue_bcast(0)
        for i, (c, k) in enumerate(pairs):
            s, w = SC[c]
            W = kw[k]
            bcr = bcr_tiles.pop(i)
            if k == 0:
                brep = bcr[:, 0:NST, :]
                crep = bcr[:, NST:32, :]
            else:
                brep = bcr[:, 0:NST, ::-1]
                crep = bcr[:, NST:32, ::-1]
            for dt in range(2):
                ns = NST if dt == 0 else 8
                if len(pending) >= 3:
                    # flush the 3-iterations-old deferred tail before its
                    # dA/dBu/Ht buffer slots are reused below
                    pending.pop(0)()
                if dt == 0 and i + 1 < len(pairs):
                    # safe point: all readers of bcr slot (i+1)%3's previous
                    # occupant (pair i-2) have been emitted by now
                    issue_bcast(i + 1)
                dct = dcs[k][dt]
                dcsl = _sl(dct[:], k, s, w)
                dxsl = _sl(dcs[k][2 + dt][:], k, s, w)
                dA = scn.tile([128, ns, LC + 1], bf16, tag="dA",
                              name=f"dA{dt}", bufs=3)
                dBu = scn.tile([128, ns, LC + 1], bf16, tag="dBu",
                               name=f"dBu{dt}", bufs=2)
                Ht = scn.tile([128, ns, LC + 1], bf16, tag="Ht",
                              name=f"Ht{dt}", bufs=3)
                nc.gpsimd.memset(dA[:, :, 0:1], 0.0)
                nc.gpsimd.tensor_copy(out=dBu[:, :, 0:1], in_=hp[k][dt][:])
                for n in range(ns):
                    nc.scalar.activation(dA[:, n, 1:], dcsl, AF.Exp,
                                         scale=W["ac"][dt][:, n:n + 1])
                beng = nc.vector if dt == 0 else nc.gpsimd
                if dt == 0:
                    beng.tensor_tensor(out=dBu[:, :, 1:],
                                       in0=_rep(dxsl, NST),
                                       in1=brep, op=OP_.mult)
                else:
                    # packed: nh half selects B rows 0:8 / 8:16
                    beng.tensor_tensor(out=dBu[0:DT1, :, 1:],
                                       in0=_rep(dxsl[0:DT1], 8),
                                       in1=brep[0:DT1, 0:8], op=OP_.mult)
                    beng.tensor_tensor(out=dBu[DT1:128, :, 1:],
                                       in0=_rep(dxsl[DT1:128], 8),
                                       in1=brep[DT1:128, 8:NST], op=OP_.mult)
                nc.vector.tensor_tensor_scan(
                    out=Ht[:].rearrange("p a b -> p (a b)"),
                    data0=dA[:].rearrange("p a b -> p (a b)"),
                    data1=dBu[:].rearrange("p a b -> p (a b)"),
                    initial=0.0, op0=OP_.mult, op1=OP_.add)
                Pt = P0 if dt == 0 else P1
                args = (dt, dA, Ht, crep, hp[k][dt], _sl(Pt[:], k, s, w))
                pending.append(lambda a=args: tail(*a))
        for fn in pending:
            fn()

        # ---- Q = mrow*P + mcol*transpose(P) ----
        Q0 = big.tile([DT0, L], bf16, tag="xs0", name="Q0")
        Q1 = big.tile([DT1, L], bf16, tag="xs1", name="Q1")
        nc.vector.tensor_scalar_mul(Q0[:], _twh(P0[:]), mcol0[:])
        nc.vector.scalar_tensor_tensor(out=Q0[:], in0=P0[:], scalar=mrow0[:],
                                       in1=Q0[:], op0=OP_.mult, op1=OP_.add)
        nc.gpsimd.tensor_scalar_mul(Q1[:], _twh(P1[:]), mcol1[:])
        qtm = big.tile([DT1, L], bf16, tag="xm1", name="qtm")
        nc.gpsimd.tensor_scalar_mul(qtm[:], P1[:], mrow1[:])
        nc.gpsimd.tensor_tensor(out=Q1[:], in0=qtm[:], in1=Q1[:], op=OP_.add)
        nc.sync.dma_start(oq_d[0:DT0], Q0[:])
        nc.sync.dma_start(oq_d[DT0:DIN], Q1[:])
    nc.compile()
    return nc


# ---------------------------------------------------------------- pass 2
def build_nc2():
    nc = bacc.Bacc("TRN2", target_bir_lowering=False, debug=False, num_devices=8)
    din = {}

    def I(name, shape, dt=f32):
        din[name] = nc.dram_tensor(name, shape, dt, kind="ExternalInput")

    I("ym", [DIN, L], bf16); I("xin", [COUT, L]); I("zin", [DIN, L], bf16)
    I("OPm", [DIN, COUT], bf16); I("OPB", [DIN, COUT], bf16)
    I("PW1", [COUT, HID], bf16); I("g1", [HID, 1]); I("bb1", [HID, 1])
    I("cbdiag0", [DT0, 9, DT0], bf16); I("cbdiag1", [DT1, 9, DT1], bf16)
    I("g2", [HID, 1]); I("bb2", [HID, 1])
    I("PW2", [HID, COUT], bf16); I("g3", [COUT, 1]); I("bb3", [COUT, 1])
    I("fw", [COUT, 1]); I("fb", [COUT, 1])
    out_d = nc.dram_tensor("o", [COUT, L], f32, kind="ExternalOutput")

    ctx = contextlib.ExitStack()
    with tile.TileContext(nc) as tc, ctx:
        const = ctx.enter_context(tc.tile_pool(name="const", bufs=1))
        big = ctx.enter_context(tc.tile_pool(name="big", bufs=1))
        work = ctx.enter_context(tc.tile_pool(name="work", bufs=2))
        psM = ctx.enter_context(tc.tile_pool(name="psM", bufs=2, space="PSUM"))

        def load2(name, rows, cols, dt=f32):
            t0 = const.tile([DT0, cols], dt, tag=name + "0", name=name + "0")
            t1 = const.tile([DT1, cols], dt, tag=name + "1", name=name + "1")
            nc.sync.dma_start(t0[:], din[name][0:DT0])
            nc.sync.dma_start(t1[:], din[name][DT0:rows])
            return t0, t1

        def load1(name, rows):
            t = const.tile([rows, 1], f32, tag=name, name=name)
            nc.sync.dma_start(t[:], din[name][:])
            return t

        # input data first so the out-norm chain isn't stuck behind consts
        ym0 = big.tile([DT0, L], bf16, tag="ym0")
        ym1 = big.tile([DT1, L], bf16, tag="ym1")
        nc.sync.dma_start(ym0[:], din["ym"][0:DT0])
        nc.sync.dma_start(ym1[:], din["ym"][DT0:DIN])
        xres = big.tile([COUT, L], f32, tag="xres")
        nc.sync.dma_start(xres[:], din["xin"][:])
        zc0 = big.tile([DT0, L], bf16, tag="zc0")
        zc1 = big.tile([DT1, L], bf16, tag="zc1")
        nc.sync.dma_start(zc0[:], din["zin"][0:DT0])
        nc.sync.dma_start(zc1[:], din["zin"][DT0:DIN])
        OP0, OP1 = load2("OPm", DIN, COUT, bf16)
        OPB0, OPB1 = load2("OPB", DIN, COUT, bf16)
        PW1t = const.tile([COUT, HID], bf16)
        nc.sync.dma_start(PW1t[:], din["PW1"][:])
        g1c0, g1c1 = load2("g1", HID, 1)
        bb1c0, bb1c1 = load2("bb1", HID, 1)
        cbd0 = const.tile([DT0, 9, DT0], bf16)
        nc.sync.dma_start(cbd0[:], din["cbdiag0"][:])
        cbd1 = const.tile([DT1, 9, DT1], bf16)
        nc.sync.dma_start(cbd1[:], din["cbdiag1"][:])
        g2c0, g2c1 = load2("g2", HID, 1)
        bb2c0, bb2c1 = load2("bb2", HID, 1)
        PW20, PW21 = load2("PW2", HID, COUT, bf16)
        g3c = load1("g3", COUT); bb3c = load1("bb3", COUT)
        fwc = load1("fw", COUT); fbc = load1("fb", COUT)
        onesb = const.tile([128, 1], bf16); nc.vector.memset(onesb[:], 1.0)
        onesrow = const.tile([1, 128], bf16); nc.vector.memset(onesrow[:], 1.0)
        epsc = const.tile([1, 1], f32); nc.vector.memset(epsc[:], EPS)

        # out-norm stats over 192 partitions (per-chunk)
        mean_r = big.tile([1, L], bf16, tag="mean")
        rs_r = big.tile([1, L], bf16, tag="rs")
        for (s, w) in MM:
            ps = psM.tile([128, MMC], f32, tag="mm", name="pso1")
            nc.tensor.matmul(ps[:1, :w], onesb[:], ym0[:, s:s + w],
                             start=True, stop=False)
            nc.tensor.matmul(ps[:1, :w], onesb[:DT1], ym1[:, s:s + w],
                             start=False, stop=True)
            nc.scalar.activation(mean_r[:, s:s + w], ps[:1, :w], AF.Copy,
                                 scale=1.0 / DIN)
            ps2 = psM.tile([128, MMC], f32, tag="mm", name="pso2")
            for i, (t, rows) in enumerate(((ym0, DT0), (ym1, DT1))):
                sq = work.tile([128, MMC], bf16, tag="sqc", bufs=1)
                nc.vector.tensor_tensor(out=sq[:rows, :w], in0=t[:, s:s + w],
                                        in1=t[:, s:s + w], op=OP_.mult)
                nc.tensor.matmul(ps2[:1, :w], onesb[:rows], sq[:rows, :w],
                                 start=(i == 0), stop=(i == 1))
            mq = work.tile([1, MMC], f32, tag="mq", bufs=1)
            nc.scalar.activation(mq[:, :w], ps2[:1, :w], AF.Copy,
                                 scale=1.0 / DIN)
            msqc = work.tile([1, MMC], f32, tag="msqc", bufs=1)
            nc.vector.tensor_tensor(out=msqc[:, :w], in0=mean_r[:, s:s + w],
                                    in1=mean_r[:, s:s + w], op=OP_.mult)
            nc.vector.tensor_tensor(out=mq[:, :w], in0=mq[:, :w],
                                    in1=msqc[:, :w], op=OP_.subtract)
            nc.scalar.activation(mq[:, :w], mq[:, :w], AF.Sqrt, bias=epsc[:])
            with nc.allow_low_precision(reason="bf16 1/std is well conditioned"):
                nc.vector.reciprocal(rs_r[:, s:s + w], mq[:, :w])

        x2f = big.tile([COUT, L], f32, tag="x2f")
        x2b = big.tile([COUT, L], bf16, tag="x2b")
        for (s, w) in MM:
            pm = psM.tile([128, MMC], f32, tag="mm", name="psm")
            nc.tensor.matmul(pm[:, :w], onesrow[:], mean_r[:, s:s + w],
                             start=True, stop=True)
            pr = psM.tile([128, MMC], f32, tag="mm", name="psr")
            nc.tensor.matmul(pr[:, :w], onesrow[:], rs_r[:, s:s + w],
                             start=True, stop=True)
            po = psM.tile([128, MMC], f32, tag="mm", name="pso")
            for i, (t, z, rows) in enumerate(((ym0, zc0, DT0), (ym1, zc1, DT1))):
                yn = work.tile([128, MMC], bf16, tag=f"yn{i}", name=f"yn{i}")
                nc.vector.tensor_tensor(out=yn[:rows, :w], in0=t[:, s:s + w],
                                        in1=pm[:rows, :w], op=OP_.subtract)
                nc.vector.tensor_tensor(out=yn[:rows, :w], in0=yn[:rows, :w],
                                        in1=pr[:rows, :w], op=OP_.mult)
                nc.vector.tensor_tensor(out=yn[:rows, :w], in0=yn[:rows, :w],
                                        in1=z[:, s:s + w], op=OP_.mult)
                OPt = OP0 if i == 0 else OP1
                OPBt = OPB0 if i == 0 else OPB1
                nc.tensor.matmul(po[:COUT, :w], OPt[:], yn[:rows, :w],
                                 start=(i == 0), stop=False)
                nc.tensor.matmul(po[:COUT, :w], OPBt[:], z[:, s:s + w],
                                 start=False, stop=(i == 1))
            nc.vector.tensor_tensor(out=x2f[:, s:s + w], in0=po[:COUT, :w],
                                    in1=xres[:, s:s + w], op=OP_.add)
            nc.scalar.activation(x2b[:, s:s + w], x2f[:, s:s + w], AF.Copy)

        # ConvBlock: PW1 + gelu
        t0 = big.tile([DT0, L], bf16, tag="ym0", name="t0")
        t1 = big.tile([DT1, L], bf16, tag="ym1", name="t1")
        for (s, w) in MM:
            for (dst, coff, rows, gc_, bc_) in ((t0, 0, DT0, g1c0, bb1c0),
                                                (t1, DT0, DT1, g1c1, bb1c1)):
                ps = psM.tile([128, MMC], f32, tag="mm", name="psp1")
                nc.tensor.matmul(ps[:rows, :w], PW1t[:, coff:coff + rows],
                                 x2b[:, s:s + w], start=True, stop=True)
                nc.scalar.activation(dst[:, s:s + w], ps[:rows, :w], AF.Gelu,
                                     bias=bc_[:], scale=gc_[:])
        # dw conv via PE; fused bn2+gelu on psum
        v0 = big.tile([DT0, L], bf16, tag="zc0", name="v0")
        v1 = big.tile([DT1, L], bf16, tag="zc1", name="v1")
        for (src, cd, rows, out, gc_, bc_) in (
                (t0, cbd0, DT0, v0, g2c0, bb2c0),
                (t1, cbd1, DT1, v1, g2c1, bb2c1)):
            pad = work.tile([128, 50, 50], bf16, tag="pad", bufs=1)
            nc.vector.memset(pad[:rows], 0.0)
            nc.vector.tensor_copy(out=pad[:rows, 1:49, 1:49], in_=_pl3(src[:]))
            for (y0, ny) in CROWS:
                pc = psM.tile([128, MMC], f32, tag="mm", name="pscv")
                for j in range(9):
                    dy, dx = divmod(j, 3)
                    view = pad[:rows, y0 + dy:y0 + dy + ny, dx:dx + 48]
                    nc.tensor.matmul(pc[:rows, :ny * 48], cd[:, j], view,
                                     start=(j == 0), stop=(j == 8))
                nc.scalar.activation(out[:, y0 * 48:(y0 + ny) * 48],
                                     pc[:rows, :ny * 48], AF.Gelu,
                                     bias=bc_[:], scale=gc_[:])
        # PW2 + bn3 + residual
        x3f = big.tile([COUT, L], f32, tag="x3f")
        x3b = big.tile([COUT, L], bf16, tag="xres", name="x3b")
        for (s, w) in MM:
            ps = psM.tile([128, MMC], f32, tag="mm", name="psp2")
            nc.tensor.matmul(ps[:COUT, :w], PW20[:], v0[:, s:s + w],
                             start=True, stop=False)
            nc.tensor.matmul(ps[:COUT, :w], PW21[:], v1[:, s:s + w],
                             start=False, stop=True)
            cbt = work.tile([128, MMC], bf16, tag="cbt", bufs=1)
            nc.scalar.activation(cbt[:COUT, :w], ps[:COUT, :w], AF.Identity,
                                 bias=bb3c[:], scale=g3c[:])
            nc.vector.tensor_tensor(out=x3f[:, s:s + w], in0=cbt[:COUT, :w],
                                    in1=x2f[:, s:s + w], op=OP_.add)
            nc.scalar.activation(x3b[:, s:s + w], x3f[:, s:s + w], AF.Copy)

        # final LN
        mean2 = big.tile([1, L], bf16, tag="mean2")
        rs2 = big.tile([1, L], bf16, tag="rs2")
        for (s, w) in MM:
            ps = psM.tile([128, MMC], f32, tag="mm", name="psf1")
            nc.tensor.matmul(ps[:1, :w], onesb[:COUT], x3b[:, s:s + w],
                             start=True, stop=True)
            nc.scalar.activation(mean2[:, s:s + w], ps[:1, :w], AF.Copy,
                                 scale=1.0 / COUT)
            sq = work.tile([128, MMC], bf16, tag="sqc", bufs=1)
            nc.vector.tensor_tensor(out=sq[:COUT, :w], in0=x3b[:, s:s + w],
                                    in1=x3b[:, s:s + w], op=OP_.mult)
            ps2 = psM.tile([128, MMC], f32, tag="mm", name="psf2")
            nc.tensor.matmul(ps2[:1, :w], onesb[:COUT], sq[:COUT, :w],
                             start=True, stop=True)
            mq2 = work.tile([1, MMC], f32, tag="mq2", bufs=1)
            nc.scalar.activation(mq2[:, :w], ps2[:1, :w], AF.Copy,
                                 scale=1.0 / COUT)
            msqc2 = work.tile([1, MMC], f32, tag="msqc2", bufs=1)
            nc.vector.tensor_tensor(out=msqc2[:, :w], in0=mean2[:, s:s + w],
                                    in1=mean2[:, s:s + w], op=OP_.mult)
            nc.vector.tensor_tensor(out=mq2[:, :w], in0=mq2[:, :w],
                                    in1=msqc2[:, :w], op=OP_.subtract)
            nc.scalar.activation(mq2[:, :w], mq2[:, :w], AF.Sqrt, bias=epsc[:])
            with nc.allow_low_precision(reason="bf16 1/std is well conditioned"):
                nc.vector.reciprocal(rs2[:, s:s + w], mq2[:, :w])
        for (s, w) in MM:
            pm = psM.tile([128, MMC], f32, tag="mm", name="psfm")
            nc.tensor.matmul(pm[:, :w], onesrow[:], mean2[:, s:s + w],
                             start=True, stop=True)
            pr = psM.tile([128, MMC], f32, tag="mm", name="psfr")
            nc.tensor.matmul(pr[:, :w], onesrow[:], rs2[:, s:s + w],
                             start=True, stop=True)
            oc = work.tile([128, MMC], f32, tag="oc", bufs=1)
            nc.vector.tensor_tensor(out=oc[:COUT, :w], in0=x3f[:, s:s + w],
                                    in1=pm[:COUT, :w], op=OP_.subtract)
            nc.vector.tensor_tensor(out=oc[:COUT, :w], in0=oc[:COUT, :w],
                                    in1=pr[:COUT, :w], op=OP_.mult)
            nc.vector.tensor_scalar(out=oc[:COUT, :w], in0=oc[:COUT, :w],
                                    scalar1=fwc[:], scalar2=fbc[:],
                                    op0=OP_.mult, op1=OP_.add)
            nc.sync.dma_start(out_d[:, s:s + w], oc[:COUT, :w])
    nc.compile()
    return nc


_NC1, _NC2 = None, None


def _get_ncs():
    global _NC1, _NC2
    if _NC1 is None:
        _NC1 = build_nc1()
        _NC2 = build_nc2()
    return _NC1, _NC2


def _bf(a):
    import jax.numpy as jnp
    return np.asarray(jnp.asarray(np.asarray(a, np.float32), jnp.bfloat16))


def _diag9(wmat, rows):
    out = np.zeros((rows, 9, rows), np.float32)
    idx = np.arange(rows)
    for j in range(9):
        out[idx, j, idx] = wmat[:, j]
    return out


def prep_pass1(ip):
    W1 = (np.diag(ip["ln1_w"]) @ ip["in_proj_W"]).astype(np.float32)
    b1 = (ip["ln1_b"] @ ip["in_proj_W"] + ip["in_proj_b"]).astype(np.float32)
    A = (-np.exp(ip["A_logs"].astype(np.float64))).astype(np.float32).reshape(KDIR, DIN, NST)
    Ds = ip["Ds"].reshape(KDIR, DIN)
    col = lambda v: np.ascontiguousarray(v.reshape(-1, 1), dtype=np.float32)
    convW = ip["conv_W"].reshape(DIN, 9)
    base = dict(projW=ip["proj_W"], projb=col(ip["proj_b"]), W1=_bf(W1),
                b1=col(b1),
                cdiag0=_bf(_diag9(convW[0:DT0], DT0)),
                cdiag1=_bf(_diag9(convW[DT0:DIN], DT1)),
                convb=col(ip["conv_b"]),
                eye=_bf(np.eye(128, dtype=np.float32)))
    # packed dt1 (channels 128:192 as p = d + 64*nh, 8 states per slot)
    cd1 = np.zeros((DT1, 9, 128), np.float32)
    di = np.arange(DT1)
    for j in range(9):
        cd1[di, j, di] = convW[DT0 + di, j]
        cd1[di, j, DT1 + di] = convW[DT0 + di, j]
    base["cdiag1d"] = _bf(cd1)
    base["cbd"] = col(np.tile(ip["conv_b"][DT0:], 2))
    base["red1"] = _bf(np.tile(np.eye(DT1, dtype=np.float32), (2, 1)))
    maps = []
    for c in range(8):
        b, plane = c // 2, c % 2
        ks = [plane, plane + 2]
        m = dict(base)
        m["xc_t"] = np.ascontiguousarray(ip["x_cat"][b].reshape(L, CIN).T)
        m["xpw"] = _bf(np.stack([ip["x_proj_W"][k].T for k in ks]))
        xpz = np.zeros((2, 128, RNK + 2 * NST), np.float32)
        for kk, k in enumerate(ks):
            xpz[kk, 0:DT1] = ip["x_proj_W"][k].T[DT0:DIN]
        m["xpz"] = _bf(xpz)
        m["dtw"] = _bf(np.stack([ip["dt_W"][k].T for k in ks]))
        m["dtwd"] = _bf(np.stack(
            [np.tile(ip["dt_W"][k].T[:, DT0:], (1, 2)) for k in ks]))
        m["dtb"] = np.ascontiguousarray(np.stack([col(ip["dt_b"][k]) for k in ks]))
        m["dtbd"] = np.ascontiguousarray(np.stack(
            [col(np.tile(ip["dt_b"][k][DT0:], 2)) for k in ks]))
        m["acoef"] = np.ascontiguousarray(np.stack([A[k] for k in ks]))
        acp = np.zeros((2, 128, 8), np.float32)
        for kk, k in enumerate(ks):
            for nh in range(2):
                acp[kk, nh * DT1:(nh + 1) * DT1, :] = A[k][DT0:DIN,
                                                           nh * 8:(nh + 1) * 8]
        m["acp"] = np.ascontiguousarray(acp)
        m["dsum"] = col(Ds[ks[0]] + Ds[ks[1]])
        m["mrow"] = np.full((DIN, 1), 1.0 - plane, np.float32)
        m["mcol"] = np.full((DIN, 1), float(plane), np.float32)
        maps.append(m)
    return maps


def prep_pass2(ip, res1):
    OPm = (np.diag(ip["out_norm_w"]) @ ip["out_proj_W"]).astype(np.float32)
    OPB = (np.diag(ip["out_norm_b"]) @ ip["out_proj_W"]).astype(np.float32)
    col = lambda v: np.ascontiguousarray(v.reshape(-1, 1), dtype=np.float32)
    cbw = ip["cb_dw_W"].reshape(HID, 9)
    base = dict(OPm=_bf(OPm), OPB=_bf(OPB),
                PW1=_bf(ip["cb_pw1_W"][:, :, 0, 0].T),
                g1=col(ip["cb_bn1_g"]), bb1=col(ip["cb_bn1_b"]),
                cbdiag0=_bf(_diag9(cbw[0:DT0], DT0)),
                cbdiag1=_bf(_diag9(cbw[DT0:HID], DT1)),
                g2=col(ip["cb_bn2_g"]), bb2=col(ip["cb_bn2_b"]),
                PW2=_bf(ip["cb_pw2_W"][:, :, 0, 0].T),
                g3=col(ip["cb_bn3_g"]), bb3=col(ip["cb_bn3_b"]),
                fw=col(ip["norm_w"]), fb=col(ip["norm_b"]))
    maps = []
    for c in range(8):
        b = c // 2
        m = dict(base)
        ymf = (np.asarray(res1[2 * b]["oq"], np.float32)
               + np.asarray(res1[2 * b + 1]["oq"], np.float32))
        m["ym"] = _bf(ymf)
        m["xin"] = np.asarray(res1[2 * b]["ox"], np.float32)
        m["zin"] = np.ascontiguousarray(res1[2 * b]["oz"])
        maps.append(m)
    return maps


def kernel(**inputs):
    ip = {k: np.asarray(v, np.float32) for k, v in inputs.items()}
    nc1, nc2 = _get_ncs()
    res1 = run_bass_kernel_spmd(nc1, prep_pass1(ip), list(range(8))).results
    res2 = run_bass_kernel_spmd(nc2, prep_pass2(ip, res1), list(range(8))).results
    outs = [np.asarray(res2[2 * b]["o"], np.float32).T.reshape(H_, W_, COUT)
            for b in range(B_)]
    return np.stack(outs).astype(np.float32)


# revision 41
# speedup vs baseline: 1.9558x; 1.0191x over previous
"""Trainium2 Bass kernel for nn_DecoderFusionBlock (VSS/Mamba decoder fusion block).

Two-pass SPMD over 8 cores:
  pass 1: core c -> batch b=c//2, plane=c%2 (row-/col-major spatial order).
          proj/LN/in_proj (f32r / bf16 matmuls), depthwise conv via PE diag
          matmuls, then the selective scan for the plane's two directions.
          bf16 data path with fp32 scan state; B/C broadcast to all channel
          partitions via a DRAM-staged broadcast DMA so the big elementwise
          multiplies run in the DVE 2x (2-byte) mode; the n-state reduction
          runs on the PE as identity-weight matmul accumulation in PSUM.
  host:   ym[b] = Q[2b] + Q[2b+1]  (the only cross-core reduction)
  pass 2: core c -> batch b=c//2: out-norm, gate, out_proj+residual,
          ConvBlock (conv again via PE), final LayerNorm.
"""

import contextlib
import numpy as np

import concourse.bass as bass
import concourse.tile as tile
from concourse import bacc, mybir
from concourse.bass_utils import run_bass_kernel_spmd

f32 = mybir.dt.float32
f32r = mybir.dt.float32r
bf16 = mybir.dt.bfloat16
AF = mybir.ActivationFunctionType
OP_ = mybir.AluOpType

B_, H_, W_ = 4, 48, 48
L = H_ * W_
CIN, COUT = 192, 96
DIN, NST, RNK, KDIR = 192, 16, 6, 4
HID = 192
EPS = 1e-5
DT0, DT1 = 128, 64
MMC = 512
MM = [(s, min(MMC, L - s)) for s in range(0, L, MMC)]
LC = 256
SC = [(i * LC, LC) for i in range(L // LC)]
CROWS = [(0, 10), (10, 10), (20, 10), (30, 10), (40, 8)]


def _rev(ap, s, w):
    hi = L - 1 - s
    lo = hi - w
    return ap[:, hi::-1] if lo < 0 else ap[:, hi:lo:-1]


def _sl(ap, k, s, w):
    return ap[:, s:s + w] if k == 0 else _rev(ap, s, w)


def _rep(a, n):
    return bass.AP(tensor=a.tensor, offset=a.offset, ap=[a.ap[0], [0, n], a.ap[1]])


def _twh(a):
    st = a.ap[1][0]
    return bass.AP(tensor=a.tensor, offset=a.offset,
                   ap=[a.ap[0], [st, 48], [48 * st, 48]])


def _pl3(a):
    st = a.ap[1][0]
    return bass.AP(tensor=a.tensor, offset=a.offset,
                   ap=[a.ap[0], [48 * st, 48], [st, 48]])


# ---------------------------------------------------------------- pass 1
def build_nc1():
    nc = bacc.Bacc("TRN2", target_bir_lowering=False, debug=False, num_devices=8)
    din = {}

    def I(name, shape, dt=f32):
        din[name] = nc.dram_tensor(name, shape, dt, kind="ExternalInput")

    I("xc_t", [CIN, L], f32r)
    I("projW", [CIN, COUT], f32r); I("projb", [COUT, 1])
    I("W1", [COUT, 2 * DIN], bf16); I("b1", [2 * DIN, 1])
    I("cdiag0", [DT0, 9, DT0], bf16); I("cdiag1", [DT1, 9, DT1], bf16)
    I("convb", [DIN, 1])
    I("eye", [128, 128], bf16)
    I("xpw", [2, DIN, RNK + 2 * NST], bf16)
    I("xpz", [2, 128, RNK + 2 * NST], bf16)
    I("dtw", [2, RNK, DIN], bf16); I("dtwd", [2, RNK, 128], bf16)
    I("dtb", [2, DIN, 1]); I("dtbd", [2, 128, 1])
    I("acoef", [2, DIN, NST]); I("acp", [2, 128, 8]); I("dsum", [DIN, 1])
    I("cdiag1d", [DT1, 9, 128], bf16); I("cbd", [128, 1])
    I("red1", [128, DT1], bf16)
    I("mrow", [DIN, 1]); I("mcol", [DIN, 1])
    oq_d = nc.dram_tensor("oq", [DIN, L], bf16, kind="ExternalOutput")
    ox_d = nc.dram_tensor("ox", [COUT, L], f32, kind="ExternalOutput")
    oz_d = nc.dram_tensor("oz", [DIN, L], bf16, kind="ExternalOutput")
    bcd = nc.dram_tensor("BCd", [2, 32, L], bf16, kind="Internal")

    ctx = contextlib.ExitStack()
    with tile.TileContext(nc) as tc, ctx:
        const = ctx.enter_context(tc.tile_pool(name="const", bufs=1))
        big = ctx.enter_context(tc.tile_pool(name="big", bufs=1))
        work = ctx.enter_context(tc.tile_pool(name="work", bufs=2))
        scn = ctx.enter_context(tc.tile_pool(name="scn", bufs=1))
        psM = ctx.enter_context(tc.tile_pool(name="psM", bufs=2, space="PSUM"))
        psY = ctx.enter_context(tc.tile_pool(name="psY", bufs=2, space="PSUM"))

        def load2(name, rows, cols, dt=f32):
            t0 = const.tile([DT0, cols], dt, tag=name + "0", name=name + "0")
            t1 = const.tile([DT1, cols], dt, tag=name + "1", name=name + "1")
            nc.sync.dma_start(t0[:], din[name][0:DT0])
            nc.sync.dma_start(t1[:], din[name][DT0:rows])
            return t0, t1

        # input data first so the proj chain isn't stuck behind const loads
        xc0 = big.tile([DT0, L], f32r, tag="xc0")
        xc1 = big.tile([DT1, L], f32r, tag="xc1")
        nc.sync.dma_start(xc0[:], din["xc_t"][0:DT0])
        nc.sync.dma_start(xc1[:], din["xc_t"][DT0:CIN])
        projW0 = const.tile([DT0, COUT], f32r)
        projW1 = const.tile([DT1, COUT], f32r)
        nc.sync.dma_start(projW0[:], din["projW"][0:DT0])
        nc.sync.dma_start(projW1[:], din["projW"][DT0:CIN])
        projb = const.tile([COUT, 1], f32)
        nc.sync.dma_start(projb[:], din["projb"][:])
        W1t = const.tile([COUT, 2 * DIN], bf16)
        nc.sync.dma_start(W1t[:], din["W1"][:])
        b1x0 = const.tile([DT0, 1], f32); nc.sync.dma_start(b1x0[:], din["b1"][0:128])
        b1x1 = const.tile([DT1, 1], f32); nc.sync.dma_start(b1x1[:], din["b1"][128:192])
        b1z0 = const.tile([DT0, 1], f32); nc.sync.dma_start(b1z0[:], din["b1"][192:320])
        b1z1 = const.tile([DT1, 1], f32); nc.sync.dma_start(b1z1[:], din["b1"][320:384])
        cdiag0 = const.tile([DT0, 9, DT0], bf16)
        nc.sync.dma_start(cdiag0[:], din["cdiag0"][:])
        cdiag1 = const.tile([DT1, 9, DT1], bf16)
        nc.sync.dma_start(cdiag1[:], din["cdiag1"][:])
        convb0, convb1 = load2("convb", DIN, 1)
        cdiag1d = const.tile([DT1, 9, 128], bf16)
        nc.sync.dma_start(cdiag1d[:], din["cdiag1d"][:])
        cbd = const.tile([128, 1], f32)
        nc.sync.dma_start(cbd[:], din["cbd"][:])
        red1 = const.tile([128, DT1], bf16)
        nc.sync.dma_start(red1[:], din["red1"][:])
        eye = const.tile([128, 128], bf16)
        nc.sync.dma_start(eye[:], din["eye"][:])
        dsum0, dsum1 = load2("dsum", DIN, 1)
        mrow0, mrow1 = load2("mrow", DIN, 1)
        mcol0, mcol1 = load2("mcol", DIN, 1)
        kw = []
        for k in range(2):
            xp0 = const.tile([DT0, RNK + 2 * NST], bf16, name=f"xp{k}0")
            xp1 = const.tile([128, RNK + 2 * NST], bf16, name=f"xp{k}1")
            nc.sync.dma_start(xp0[:], din["xpw"][k, 0:DT0])
            nc.sync.dma_start(xp1[:], din["xpz"][k])
            dtw = const.tile([38, DIN], bf16, tag="dtwm", name=f"dtw{k}",
                             bufs=1) if k == 0 else kw[0]["dtwt"]
            nc.sync.dma_start(dtw[k * 32:k * 32 + RNK], din["dtw"][k])
            dtwd = const.tile([38, 128], bf16, tag="dtwdm", name=f"dtwd{k}",
                              bufs=1) if k == 0 else kw[0]["dtwdt"]
            nc.sync.dma_start(dtwd[k * 32:k * 32 + RNK], din["dtwd"][k])
            dtb0 = const.tile([DT0, 1], f32, name=f"dtb{k}0")
            dtb1 = const.tile([128, 1], f32, name=f"dtb{k}1")
            nc.sync.dma_start(dtb0[:], din["dtb"][k, 0:DT0])
            nc.sync.dma_start(dtb1[:], din["dtbd"][k])
            ac0 = const.tile([DT0, NST], f32, name=f"ac{k}0")
            ac1 = const.tile([128, 8], f32, name=f"ac{k}1")
            nc.sync.dma_start(ac0[:], din["acoef"][k, 0:DT0])
            nc.sync.dma_start(ac1[:], din["acp"][k])
            kw.append(dict(xp=(xp0, xp1), dtwt=dtw, dtwdt=dtwd,
                           dtw=dtw[k * 32:k * 32 + RNK],
                           dtwd=dtwd[k * 32:k * 32 + RNK],
                           dtb=(dtb0, dtb1),
                           ac=(ac0, ac1)))

        ones128 = const.tile([128, 1], f32); nc.vector.memset(ones128[:], 1.0)
        onesrow = const.tile([1, 128], bf16); nc.vector.memset(onesrow[:], 1.0)
        epsc = const.tile([1, 1], f32); nc.vector.memset(epsc[:], EPS)

        # ---- proj (f32r matmuls, x_t kept fp32 for residual) ----
        x_t = big.tile([COUT, L], f32, tag="x_t")
        for (s, w) in MM:
            ps = psM.tile([128, MMC], f32, tag="mm", name="psproj")
            nc.tensor.matmul(ps[:COUT, :w], projW0[:], xc0[:, s:s + w],
                             start=True, stop=False)
            nc.tensor.matmul(ps[:COUT, :w], projW1[:], xc1[:, s:s + w],
                             start=False, stop=True)
            nc.scalar.activation(x_t[:, s:s + w], ps[:COUT, :w], AF.Identity,
                                 bias=projb[:])
        nc.sync.dma_start(ox_d[:], x_t[:])

        # ---- LN1 (Copy + Sqrt share the act-table phase) -> xn bf16 ----
        xn_t = big.tile([COUT, L], bf16, tag="xn")
        for (s, w) in MM:
            ps1 = psM.tile([128, MMC], f32, tag="mm", name="pss1")
            nc.tensor.matmul(ps1[:1, :w], ones128[:COUT], x_t[:, s:s + w],
                             start=True, stop=True)
            mrw = work.tile([1, MMC], bf16, tag="mrw", bufs=1)
            nc.scalar.activation(mrw[:, :w], ps1[:1, :w], AF.Copy, scale=1.0 / COUT)
            sq = work.tile([128, MMC], f32, tag="sqc", bufs=1)
            nc.vector.tensor_tensor(out=sq[:COUT, :w], in0=x_t[:, s:s + w],
                                    in1=x_t[:, s:s + w], op=OP_.mult)
            ps2 = psM.tile([128, MMC], f32, tag="mm", name="pss2")
            nc.tensor.matmul(ps2[:1, :w], ones128[:COUT], sq[:COUT, :w],
                             start=True, stop=True)
            mq = work.tile([1, MMC], f32, tag="mq", bufs=1)
            nc.scalar.activation(mq[:, :w], ps2[:1, :w], AF.Copy, scale=1.0 / COUT)
            msq = work.tile([1, MMC], f32, tag="msq", bufs=1)
            nc.vector.tensor_tensor(out=msq[:, :w], in0=mrw[:, :w],
                                    in1=mrw[:, :w], op=OP_.mult)
            nc.vector.tensor_tensor(out=mq[:, :w], in0=mq[:, :w],
                                    in1=msq[:, :w], op=OP_.subtract)
            nc.scalar.activation(mq[:, :w], mq[:, :w], AF.Sqrt, bias=epsc[:])
            rsw = work.tile([1, MMC], bf16, tag="rsw", bufs=1)
            with nc.allow_low_precision(reason="bf16 1/std is well conditioned"):
                nc.vector.reciprocal(rsw[:, :w], mq[:, :w])
            pm = psM.tile([128, MMC], f32, tag="mm", name="psbm")
            nc.tensor.matmul(pm[:, :w], onesrow[:], mrw[:, :w],
                             start=True, stop=True)
            pr = psM.tile([128, MMC], f32, tag="mm", name="psbr")
            nc.tensor.matmul(pr[:, :w], onesrow[:], rsw[:, :w],
                             start=True, stop=True)
            xn_ = work.tile([128, MMC], bf16, tag="xn_", bufs=1)
            nc.vector.tensor_tensor(out=xn_[:COUT, :w], in0=x_t[:, s:s + w],
                                    in1=pm[:COUT, :w], op=OP_.subtract)
            nc.vector.tensor_tensor(out=xn_t[:, s:s + w], in0=xn_[:COUT, :w],
                                    in1=pr[:COUT, :w], op=OP_.mult)

        # ---- in_proj (xm tiles bf16; z silu'd -> DRAM bf16) ----
        xm0 = big.tile([DT0, L], bf16, tag="xm0")
        xm1 = big.tile([DT1, L], bf16, tag="xm1")
        for (s, w) in MM:
            for (coff, rows, bcol, dst, zoff) in (
                    (0, DT0, b1x0, xm0, None), (DT0, DT1, b1x1, xm1, None),
                    (DIN, DT0, b1z0, None, 0), (DIN + DT0, DT1, b1z1, None, DT0)):
                psi = psM.tile([128, MMC], f32, tag="mm", name="psip")
                nc.tensor.matmul(psi[:rows, :w], W1t[:, coff:coff + rows],
                                 xn_t[:, s:s + w], start=True, stop=True)
                if dst is not None:
                    nc.scalar.activation(dst[:, s:s + w], psi[:rows, :w],
                                         AF.Identity, bias=bcol[:])
                else:
                    zc = work.tile([128, MMC], bf16, tag="zc", bufs=1)
                    nc.scalar.activation(zc[:rows, :w], psi[:rows, :w], AF.Silu,
                                         bias=bcol[:])
                    nc.sync.dma_start(oz_d[zoff:zoff + rows, s:s + w], zc[:rows, :w])

        # ---- depthwise conv via PE diag matmuls + fused SiLU; the
        #      transposed copy for the plane transform happens per row-chunk
        xs0 = big.tile([DT0, L], bf16, tag="xs0")
        xs1 = big.tile([128, L], bf16, tag="xs1")
        xt0 = big.tile([DT0, L], bf16, tag="xm0", name="xt0")
        xt1 = big.tile([128, L], bf16, tag="xm1", name="xt1")
        for (src, cd, srows, orows, out, bcol, xtt, teng) in (
                (xm0, cdiag0, DT0, DT0, xs0, convb0, xt0, nc.vector),
                (xm1, cdiag1d, DT1, 128, xs1, cbd, xt1, nc.vector)):
            pad = work.tile([128, 50, 50], bf16, tag="pad", bufs=1)
            nc.vector.memset(pad[:srows], 0.0)
            r0 = 0
            for (s, w) in MM:
                r1 = (s + w) // 48
                nc.vector.tensor_copy(out=pad[:srows, 1 + r0:1 + r1, 1:49],
                                      in_=_pl3(src[:])[:, r0:r1, :])
                r0 = r1
            for (y0, ny) in CROWS:
                pc = psM.tile([128, MMC], f32, tag="mm", name="pscv")
                for j in range(9):
                    dy, dx = divmod(j, 3)
                    view = pad[:srows, y0 + dy:y0 + dy + ny, dx:dx + 48]
                    nc.tensor.matmul(pc[:orows, :ny * 48], cd[:, j], view,
                                     start=(j == 0), stop=(j == 8))
                nc.scalar.activation(out[:, y0 * 48:(y0 + ny) * 48],
                                     pc[:orows, :ny * 48], AF.Silu, bias=bcol[:])
                # xt[p, x, y] = xs[p, y, x] for this y-chunk
                xin = _pl3(out[:])[:, y0:y0 + ny, :]
                xout = bass.AP(tensor=xtt.tensor, offset=xtt[:].offset + y0,
                               ap=[xtt[:].ap[0], [1, ny], [48, 48]])
                teng.tensor_copy(out=xout, in_=xin)
        nc.vector.tensor_scalar_mul(xs0[:], xs0[:], mrow0[:])
        nc.vector.scalar_tensor_tensor(out=xs0[:], in0=xt0[:], scalar=mcol0[:],
                                       in1=xs0[:], op0=OP_.mult, op1=OP_.add)
        nc.vector.tensor_scalar_mul(xs1[:], xs1[:], mrow0[:])
        nc.vector.scalar_tensor_tensor(out=xs1[:], in0=xt1[:], scalar=mcol0[:],
                                       in1=xs1[:], op0=OP_.mult, op1=OP_.add)

        # ---- U96: x_dbl for both k; B/C staged to DRAM bf16 ----
        u96m = big.tile([38, L], bf16, tag="u96r")
        u96rs = (u96m[0:RNK], u96m[32:38])
        for k in range(2):
            W = kw[k]
            rb = k * 32          # rank section base: 0 (k0) / 32 (k1)
            bb = rb + 32
            cb = 64 if k == 0 else 0
            for (s, w) in MM:
                ps = psM.tile([128, MMC], f32, tag="mm", name="psU")
                for (coff, ubase, m) in ((0, rb, RNK), (RNK, bb, NST),
                                         (RNK + NST, cb, NST)):
                    nc.tensor.matmul(ps[ubase:ubase + m, :w],
                                     W["xp"][0][:, coff:coff + m],
                                     xs0[:, s:s + w], start=True, stop=False)
                    nc.tensor.matmul(ps[ubase:ubase + m, :w],
                                     W["xp"][1][:, coff:coff + m],
                                     xs1[:, s:s + w], start=False, stop=True)
                nc.scalar.activation(u96rs[k][:, s:s + w],
                                     ps[rb:rb + RNK, :w], AF.Copy)
                bcsw = work.tile([112, MMC], bf16, tag="bcsw")
                nc.scalar.activation(bcsw[bb:bb + NST, :w], ps[bb:bb + NST, :w],
                                     AF.Copy)
                nc.scalar.activation(bcsw[cb:cb + NST, :w], ps[cb:cb + NST, :w],
                                     AF.Copy)
                nc.sync.dma_start(bcd[k, 0:NST, s:s + w], bcsw[bb:bb + NST, :w])
                nc.sync.dma_start(bcd[k, NST:32, s:s + w], bcsw[cb:cb + NST, :w])

        # ---- delta (Softplus) for both k, both dt ----
        dcs = []
        for k in range(2):
            W = kw[k]
            dc0 = big.tile([DT0, L], bf16, tag="xc0" if k == 0 else "xc1",
                           name=f"dc{k}0")
            dc1 = big.tile([128, L], bf16, tag=f"dc{k}1", name=f"dc{k}1")
            for (s, w) in MM:
                for dt, (rows, stat, dct) in enumerate(
                        ((DT0, W["dtw"][:, 0:DT0], dc0),
                         (128, W["dtwd"], dc1))):
                    ps = psM.tile([128, MMC], f32, tag="mm", name="psdt")
                    nc.tensor.matmul(ps[:rows, :w], stat,
                                     u96rs[k][:, s:s + w],
                                     start=True, stop=True)
                    nc.scalar.activation(dct[:, s:s + w], ps[:rows, :w],
                                         AF.Exp, bias=W["dtb"][dt][:])
            # in-place Ln (softplus) + dxc = delta*x, per chunk
            dx0 = big.tile([DT0, L], bf16, tag="x_t" if k == 0 else "xn",
                           name=f"dx{k}0")
            dx1 = big.tile([128, L], bf16, tag="xm1" if k == 0 else "u96r",
                           name=f"dx{k}1")
            for (s, w) in MM:
                for dct, dxt, xst in ((dc0, dx0, xs0), (dc1, dx1, xs1)):
                    nc.scalar.activation(dct[:, s:s + w], dct[:, s:s + w],
                                         AF.Ln, bias=1.0)
                    nc.vector.tensor_tensor(out=dxt[:, s:s + w],
                                            in0=dct[:, s:s + w],
                                            in1=xst[:, s:s + w], op=OP_.mult)
            dcs.append((dc0, dc1, dx0, dx1))

        # ---- P init with the direction-independent D term ----
        P0 = big.tile([DT0, L], bf16, tag="xm0", name="P0")
        P1 = big.tile([DT1, L], bf16, tag="xt1m", name="P1")
        nc.vector.tensor_scalar_mul(P0[:], xs0[:], dsum0[:])
        nc.gpsimd.tensor_scalar_mul(P1[:], xs1[0:DT1], dsum1[:])

        # ---- selective scan ----
        hp = [[scn.tile([128, NST if dt == 0 else 8, 1], bf16,
                        tag=f"hp{k}{dt}", name=f"hp{k}{dt}")
               for dt in range(2)] for k in range(2)]
        for k in range(2):
            for dt in range(2):
                nc.vector.memset(hp[k][dt][:], 0.0)

        pairs = [(c, k) for c in range(len(SC)) for k in range(2)]
        bcr_tiles = {}

        def issue_bcast(i):
            c, k = pairs[i]
            s, w = SC[c]
            s0 = s if k == 0 else L - s - w
            t = scn.tile([128, 32, LC], bf16, tag="bcr", bufs=3,
                         name=f"bcr{i}")
            src = bass.AP(tensor=bcd, offset=k * 32 * L + s0,
                          ap=[[0, 128], [L, 32], [1, w]])
            nc.sync.dma_start(t[:], src)
            bcr_tiles[i] = t

        def tail(dt, dA, Ht, crep, hpt, pdst):
            # after the scan: save carry, G = H*C, PE n-reduction, P +=
            nc.gpsimd.tensor_copy(out=hpt[:], in_=Ht[:, :, LC:LC + 1])
            psy = psY.tile([128, LC], f32, tag="psy", name="psy")
            if dt == 0:
                nc.vector.tensor_tensor(out=dA[:, :, 1:], in0=Ht[:, :, 1:],
                                        in1=crep, op=OP_.mult)
                for n in range(NST):
                    nc.tensor.matmul(psy[:, :], eye[:, :],
                                     dA[:, n, 1:],
                                     start=(n == 0), stop=(n == NST - 1))
                nc.vector.tensor_tensor(out=pdst, in0=psy[:DT0, :], in1=pdst,
                                        op=OP_.add)
            else:
                nc.vector.tensor_tensor(out=dA[0:DT1, :, 1:],
                                        in0=Ht[0:DT1, :, 1:],
                                        in1=crep[0:DT1, 0:8], op=OP_.mult)
                nc.vector.tensor_tensor(out=dA[DT1:128, :, 1:],
                                        in0=Ht[DT1:128, :, 1:],
                                        in1=crep[DT1:128, 8:NST], op=OP_.mult)
                for n in range(8):
                    nc.tensor.matmul(psy[:DT1, :], red1[:, :],
                                     dA[:, n, 1:],
                                     start=(n == 0), stop=(n == 7))
                nc.vector.tensor_tensor(out=pdst, in0=psy[:DT1, :], in1=pdst,
                                        op=OP_.add)

        pending = []
        issue_bcast(0)
        for i, (c, k) in enumerate(pairs):
            s, w = SC[c]
            W = kw[k]
            bcr = bcr_tiles.pop(i)
            if k == 0:
                brep = bcr[:, 0:NST, :]
                crep = bcr[:, NST:32, :]
            else:
                brep = bcr[:, 0:NST, ::-1]
                crep = bcr[:, NST:32, ::-1]
            for dt in range(2):
                ns = NST if dt == 0 else 8
                if len(pending) >= 3:
                    # flush the 3-iterations-old deferred tail before its
                    # dA/dBu/Ht buffer slots are reused below
                    pending.pop(0)()
                if dt == 0 and i + 1 < len(pairs):
                    # safe point: all readers of bcr slot (i+1)%3's previous
                    # occupant (pair i-2) have been emitted by now
                    issue_bcast(i + 1)
                dct = dcs[k][dt]
                dcsl = _sl(dct[:], k, s, w)
                dxsl = _sl(dcs[k][2 + dt][:], k, s, w)
                dA = scn.tile([128, ns, LC + 1], bf16, tag="dA",
                              name=f"dA{dt}", bufs=3)
                dBu = scn.tile([128, ns, LC + 1], bf16, tag="dBu",
                               name=f"dBu{dt}", bufs=2)
                Ht = scn.tile([128, ns, LC + 1], bf16, tag="Ht",
                              name=f"Ht{dt}", bufs=3)
                nc.gpsimd.memset(dA[:, :, 0:1], 0.0)
                nc.gpsimd.tensor_copy(out=dBu[:, :, 0:1], in_=hp[k][dt][:])
                for n in range(ns):
                    nc.scalar.activation(dA[:, n, 1:], dcsl, AF.Exp,
                                         scale=W["ac"][dt][:, n:n + 1])
                beng = nc.vector if dt == 0 else nc.gpsimd
                if dt == 0:
                    beng.tensor_tensor(out=dBu[:, :, 1:],
                                       in0=_rep(dxsl, NST),
                                       in1=brep, op=OP_.mult)
                else:
                    # packed: nh half selects B rows 0:8 / 8:16
                    beng.tensor_tensor(out=dBu[0:DT1, :, 1:],
                                       in0=_rep(dxsl[0:DT1], 8),
                                       in1=brep[0:DT1, 0:8], op=OP_.mult)
                    beng.tensor_tensor(out=dBu[DT1:128, :, 1:],
                                       in0=_rep(dxsl[DT1:128], 8),
                                       in1=brep[DT1:128, 8:NST], op=OP_.mult)
                nc.vector.tensor_tensor_scan(
                    out=Ht[:].rearrange("p a b -> p (a b)"),
                    data0=dA[:].rearrange("p a b -> p (a b)"),
                    data1=dBu[:].rearrange("p a b -> p (a b)"),
                    initial=0.0, op0=OP_.mult, op1=OP_.add)
                Pt = P0 if dt == 0 else P1
                args = (dt, dA, Ht, crep, hp[k][dt], _sl(Pt[:], k, s, w))
                pending.append(lambda a=args: tail(*a))
        for fn in pending:
            fn()

        # ---- Q = mrow*P + mcol*transpose(P) ----
        Q0 = big.tile([DT0, L], bf16, tag="xs0", name="Q0")
        Q1 = big.tile([DT1, L], bf16, tag="xs1", name="Q1")
        nc.vector.tensor_scalar_mul(Q0[:], _twh(P0[:]), mcol0[:])
        nc.vector.scalar_tensor_tensor(out=Q0[:], in0=P0[:], scalar=mrow0[:],
                                       in1=Q0[:], op0=OP_.mult, op1=OP_.add)
        nc.gpsimd.tensor_scalar_mul(Q1[:], _twh(P1[:]), mcol1[:])
        qtm = big.tile([DT1, L], bf16, tag="xm1", name="qtm")
        nc.gpsimd.tensor_scalar_mul(qtm[:], P1[:], mrow1[:])
        nc.gpsimd.tensor_tensor(out=Q1[:], in0=qtm[:], in1=Q1[:], op=OP_.add)
        nc.sync.dma_start(oq_d[0:DT0], Q0[:])
        nc.sync.dma_start(oq_d[DT0:DIN], Q1[:])
    nc.compile()
    return nc


# ---------------------------------------------------------------- pass 2
def build_nc2():
    nc = bacc.Bacc("TRN2", target_bir_lowering=False, debug=False, num_devices=8)
    din = {}

    def I(name, shape, dt=f32):
        din[name] = nc.dram_tensor(name, shape, dt, kind="ExternalInput")

    I("ym", [DIN, L], bf16); I("xin", [COUT, L]); I("zin", [DIN, L], bf16)
    I("OPm", [DIN, COUT], bf16); I("OPB", [DIN, COUT], bf16)
    I("PW1", [COUT, HID], bf16); I("g1", [HID, 1]); I("bb1", [HID, 1])
    I("cbdiag0", [DT0, 9, DT0], bf16); I("cbdiag1", [DT1, 9, DT1], bf16)
    I("g2", [HID, 1]); I("bb2", [HID, 1])
    I("PW2", [HID, COUT], bf16); I("g3", [COUT, 1]); I("bb3", [COUT, 1])
    I("fw", [COUT, 1]); I("fb", [COUT, 1])
    out_d = nc.dram_tensor("o", [COUT, L], f32, kind="ExternalOutput")

    ctx = contextlib.ExitStack()
    with tile.TileContext(nc) as tc, ctx:
        const = ctx.enter_context(tc.tile_pool(name="const", bufs=1))
        big = ctx.enter_context(tc.tile_pool(name="big", bufs=1))
        work = ctx.enter_context(tc.tile_pool(name="work", bufs=2))
        psM = ctx.enter_context(tc.tile_pool(name="psM", bufs=2, space="PSUM"))

        def load2(name, rows, cols, dt=f32):
            t0 = const.tile([DT0, cols], dt, tag=name + "0", name=name + "0")
            t1 = const.tile([DT1, cols], dt, tag=name + "1", name=name + "1")
            nc.sync.dma_start(t0[:], din[name][0:DT0])
            nc.sync.dma_start(t1[:], din[name][DT0:rows])
            return t0, t1

        def load1(name, rows):
            t = const.tile([rows, 1], f32, tag=name, name=name)
            nc.sync.dma_start(t[:], din[name][:])
            return t

        # input data first so the out-norm chain isn't stuck behind consts
        ym0 = big.tile([DT0, L], bf16, tag="ym0")
        ym1 = big.tile([DT1, L], bf16, tag="ym1")
        nc.sync.dma_start(ym0[:], din["ym"][0:DT0])
        nc.sync.dma_start(ym1[:], din["ym"][DT0:DIN])
        xres = big.tile([COUT, L], f32, tag="xres")
        nc.sync.dma_start(xres[:], din["xin"][:])
        zc0 = big.tile([DT0, L], bf16, tag="zc0")
        zc1 = big.tile([DT1, L], bf16, tag="zc1")
        nc.sync.dma_start(zc0[:], din["zin"][0:DT0])
        nc.sync.dma_start(zc1[:], din["zin"][DT0:DIN])
        OP0, OP1 = load2("OPm", DIN, COUT, bf16)
        OPB0, OPB1 = load2("OPB", DIN, COUT, bf16)
        PW1t = const.tile([COUT, HID], bf16)
        nc.sync.dma_start(PW1t[:], din["PW1"][:])
        g1c0, g1c1 = load2("g1", HID, 1)
        bb1c0, bb1c1 = load2("bb1", HID, 1)
        cbd0 = const.tile([DT0, 9, DT0], bf16)
        nc.sync.dma_start(cbd0[:], din["cbdiag0"][:])
        cbd1 = const.tile([DT1, 9, DT1], bf16)
        nc.sync.dma_start(cbd1[:], din["cbdiag1"][:])
        g2c0, g2c1 = load2("g2", HID, 1)
        bb2c0, bb2c1 = load2("bb2", HID, 1)
        PW20, PW21 = load2("PW2", HID, COUT, bf16)
        g3c = load1("g3", COUT); bb3c = load1("bb3", COUT)
        fwc = load1("fw", COUT); fbc = load1("fb", COUT)
        onesb = const.tile([128, 1], bf16); nc.vector.memset(onesb[:], 1.0)
        onesrow = const.tile([1, 128], bf16); nc.vector.memset(onesrow[:], 1.0)
        epsc = const.tile([1, 1], f32); nc.vector.memset(epsc[:], EPS)

        # out-norm stats over 192 partitions (per-chunk)
        mean_r = big.tile([1, L], bf16, tag="mean")
        rs_r = big.tile([1, L], bf16, tag="rs")
        for (s, w) in MM:
            ps = psM.tile([128, MMC], f32, tag="mm", name="pso1")
            nc.tensor.matmul(ps[:1, :w], onesb[:], ym0[:, s:s + w],
                             start=True, stop=False)
            nc.tensor.matmul(ps[:1, :w], onesb[:DT1], ym1[:, s:s + w],
                             start=False, stop=True)
            nc.scalar.activation(mean_r[:, s:s + w], ps[:1, :w], AF.Copy,
                                 scale=1.0 / DIN)
            ps2 = psM.tile([128, MMC], f32, tag="mm", name="pso2")
            for i, (t, rows) in enumerate(((ym0, DT0), (ym1, DT1))):
                sq = work.tile([128, MMC], bf16, tag="sqc", bufs=1)
                nc.vector.tensor_tensor(out=sq[:rows, :w], in0=t[:, s:s + w],
                                        in1=t[:, s:s + w], op=OP_.mult)
                nc.tensor.matmul(ps2[:1, :w], onesb[:rows], sq[:rows, :w],
                                 start=(i == 0), stop=(i == 1))
            mq = work.tile([1, MMC], f32, tag="mq", bufs=1)
            nc.scalar.activation(mq[:, :w], ps2[:1, :w], AF.Copy,
                                 scale=1.0 / DIN)
            msqc = work.tile([1, MMC], f32, tag="msqc", bufs=1)
            nc.vector.tensor_tensor(out=msqc[:, :w], in0=mean_r[:, s:s + w],
                                    in1=mean_r[:, s:s + w], op=OP_.mult)
            nc.vector.tensor_tensor(out=mq[:, :w], in0=mq[:, :w],
                                    in1=msqc[:, :w], op=OP_.subtract)
            nc.scalar.activation(mq[:, :w], mq[:, :w], AF.Sqrt, bias=epsc[:])
            with nc.allow_low_precision(reason="bf16 1/std is well conditioned"):
                nc.vector.reciprocal(rs_r[:, s:s + w], mq[:, :w])

        x2f = big.tile([COUT, L], f32, tag="x2f")
        x2b = big.tile([COUT, L], bf16, tag="x2b")
        for (s, w) in MM:
            pm = psM.tile([128, MMC], f32, tag="mm", name="psm")
            nc.tensor.matmul(pm[:, :w], onesrow[:], mean_r[:, s:s + w],
                             start=True, stop=True)
            pr = psM.tile([128, MMC], f32, tag="mm", name="psr")
            nc.tensor.matmul(pr[:, :w], onesrow[:], rs_r[:, s:s + w],
                             start=True, stop=True)
            po = psM.tile([128, MMC], f32, tag="mm", name="pso")
            for i, (t, z, rows) in enumerate(((ym0, zc0, DT0), (ym1, zc1, DT1))):
                yn = work.tile([128, MMC], bf16, tag=f"yn{i}", name=f"yn{i}")
                nc.vector.tensor_tensor(out=yn[:rows, :w], in0=t[:, s:s + w],
                                        in1=pm[:rows, :w], op=OP_.subtract)
                nc.vector.tensor_tensor(out=yn[:rows, :w], in0=yn[:rows, :w],
                                        in1=pr[:rows, :w], op=OP_.mult)
                nc.vector.tensor_tensor(out=yn[:rows, :w], in0=yn[:rows, :w],
                                        in1=z[:, s:s + w], op=OP_.mult)
                OPt = OP0 if i == 0 else OP1
                OPBt = OPB0 if i == 0 else OPB1
                nc.tensor.matmul(po[:COUT, :w], OPt[:], yn[:rows, :w],
                                 start=(i == 0), stop=False)
                nc.tensor.matmul(po[:COUT, :w], OPBt[:], z[:, s:s + w],
                                 start=False, stop=(i == 1))
            nc.vector.tensor_tensor(out=x2f[:, s:s + w], in0=po[:COUT, :w],
                                    in1=xres[:, s:s + w], op=OP_.add)
            nc.scalar.activation(x2b[:, s:s + w], x2f[:, s:s + w], AF.Copy)

        # ConvBlock: PW1 + gelu
        t0 = big.tile([DT0, L], bf16, tag="ym0", name="t0")
        t1 = big.tile([DT1, L], bf16, tag="ym1", name="t1")
        for (s, w) in MM:
            for (dst, coff, rows, gc_, bc_) in ((t0, 0, DT0, g1c0, bb1c0),
                                                (t1, DT0, DT1, g1c1, bb1c1)):
                ps = psM.tile([128, MMC], f32, tag="mm", name="psp1")
                nc.tensor.matmul(ps[:rows, :w], PW1t[:, coff:coff + rows],
                                 x2b[:, s:s + w], start=True, stop=True)
                nc.scalar.activation(dst[:, s:s + w], ps[:rows, :w], AF.Gelu,
                                     bias=bc_[:], scale=gc_[:])
        # dw conv via PE; fused bn2+gelu on psum
        v0 = big.tile([DT0, L], bf16, tag="zc0", name="v0")
        v1 = big.tile([DT1, L], bf16, tag="zc1", name="v1")
        for (src, cd, rows, out, gc_, bc_) in (
                (t0, cbd0, DT0, v0, g2c0, bb2c0),
                (t1, cbd1, DT1, v1, g2c1, bb2c1)):
            pad = work.tile([128, 50, 50], bf16, tag="pad", bufs=1)
            nc.vector.memset(pad[:rows], 0.0)
            r0 = 0
            for (s, w) in MM:
                r1 = (s + w) // 48
                nc.vector.tensor_copy(out=pad[:rows, 1 + r0:1 + r1, 1:49],
                                      in_=_pl3(src[:])[:, r0:r1, :])
                r0 = r1
            for (y0, ny) in CROWS:
                pc = psM.tile([128, MMC], f32, tag="mm", name="pscv")
                for j in range(9):
                    dy, dx = divmod(j, 3)
                    view = pad[:rows, y0 + dy:y0 + dy + ny, dx:dx + 48]
                    nc.tensor.matmul(pc[:rows, :ny * 48], cd[:, j], view,
                                     start=(j == 0), stop=(j == 8))
                nc.scalar.activation(out[:, y0 * 48:(y0 + ny) * 48],
                                     pc[:rows, :ny * 48], AF.Gelu,
                                     bias=bc_[:], scale=gc_[:])
        # PW2 + bn3 + residual
        x3f = big.tile([COUT, L], f32, tag="x3f")
        x3b = big.tile([COUT, L], bf16, tag="xres", name="x3b")
        for (s, w) in MM:
            ps = psM.tile([128, MMC], f32, tag="mm", name="psp2")
            nc.tensor.matmul(ps[:COUT, :w], PW20[:], v0[:, s:s + w],
                             start=True, stop=False)
            nc.tensor.matmul(ps[:COUT, :w], PW21[:], v1[:, s:s + w],
                             start=False, stop=True)
            cbt = work.tile([128, MMC], bf16, tag="cbt", bufs=1)
            nc.scalar.activation(cbt[:COUT, :w], ps[:COUT, :w], AF.Identity,
                                 bias=bb3c[:], scale=g3c[:])
            nc.vector.tensor_tensor(out=x3f[:, s:s + w], in0=cbt[:COUT, :w],
                                    in1=x2f[:, s:s + w], op=OP_.add)
            nc.scalar.activation(x3b[:, s:s + w], x3f[:, s:s + w], AF.Copy)

        # final LN
        mean2 = big.tile([1, L], bf16, tag="mean2")
        rs2 = big.tile([1, L], bf16, tag="rs2")
        for (s, w) in MM:
            ps = psM.tile([128, MMC], f32, tag="mm", name="psf1")
            nc.tensor.matmul(ps[:1, :w], onesb[:COUT], x3b[:, s:s + w],
                             start=True, stop=True)
            nc.scalar.activation(mean2[:, s:s + w], ps[:1, :w], AF.Copy,
                                 scale=1.0 / COUT)
            sq = work.tile([128, MMC], bf16, tag="sqc", bufs=1)
            nc.vector.tensor_tensor(out=sq[:COUT, :w], in0=x3b[:, s:s + w],
                                    in1=x3b[:, s:s + w], op=OP_.mult)
            ps2 = psM.tile([128, MMC], f32, tag="mm", name="psf2")
            nc.tensor.matmul(ps2[:1, :w], onesb[:COUT], sq[:COUT, :w],
                             start=True, stop=True)
            mq2 = work.tile([1, MMC], f32, tag="mq2", bufs=1)
            nc.scalar.activation(mq2[:, :w], ps2[:1, :w], AF.Copy,
                                 scale=1.0 / COUT)
            msqc2 = work.tile([1, MMC], f32, tag="msqc2", bufs=1)
            nc.vector.tensor_tensor(out=msqc2[:, :w], in0=mean2[:, s:s + w],
                                    in1=mean2[:, s:s + w], op=OP_.mult)
            nc.vector.tensor_tensor(out=mq2[:, :w], in0=mq2[:, :w],
                                    in1=msqc2[:, :w], op=OP_.subtract)
            nc.scalar.activation(mq2[:, :w], mq2[:, :w], AF.Sqrt, bias=epsc[:])
            with nc.allow_low_precision(reason="bf16 1/std is well conditioned"):
                nc.vector.reciprocal(rs2[:, s:s + w], mq2[:, :w])
        for (s, w) in MM:
            pm = psM.tile([128, MMC], f32, tag="mm", name="psfm")
            nc.tensor.matmul(pm[:, :w], onesrow[:], mean2[:, s:s + w],
                             start=True, stop=True)
            pr = psM.tile([128, MMC], f32, tag="mm", name="psfr")
            nc.tensor.matmul(pr[:, :w], onesrow[:], rs2[:, s:s + w],
                             start=True, stop=True)
            oc = work.tile([128, MMC], f32, tag="oc", bufs=1)
            nc.vector.tensor_tensor(out=oc[:COUT, :w], in0=x3f[:, s:s + w],
                                    in1=pm[:COUT, :w], op=OP_.subtract)
            nc.vector.tensor_tensor(out=oc[:COUT, :w], in0=oc[:COUT, :w],
                                    in1=pr[:COUT, :w], op=OP_.mult)
            nc.vector.tensor_scalar(out=oc[:COUT, :w], in0=oc[:COUT, :w],
                                    scalar1=fwc[:], scalar2=fbc[:],
                                    op0=OP_.mult, op1=OP_.add)
            nc.sync.dma_start(out_d[:, s:s + w], oc[:COUT, :w])
    nc.compile()
    return nc


_NC1, _NC2 = None, None


def _get_ncs():
    global _NC1, _NC2
    if _NC1 is None:
        _NC1 = build_nc1()
        _NC2 = build_nc2()
    return _NC1, _NC2


def _bf(a):
    import jax.numpy as jnp
    return np.asarray(jnp.asarray(np.asarray(a, np.float32), jnp.bfloat16))


def _diag9(wmat, rows):
    out = np.zeros((rows, 9, rows), np.float32)
    idx = np.arange(rows)
    for j in range(9):
        out[idx, j, idx] = wmat[:, j]
    return out


def prep_pass1(ip):
    W1 = (np.diag(ip["ln1_w"]) @ ip["in_proj_W"]).astype(np.float32)
    b1 = (ip["ln1_b"] @ ip["in_proj_W"] + ip["in_proj_b"]).astype(np.float32)
    A = (-np.exp(ip["A_logs"].astype(np.float64))).astype(np.float32).reshape(KDIR, DIN, NST)
    Ds = ip["Ds"].reshape(KDIR, DIN)
    col = lambda v: np.ascontiguousarray(v.reshape(-1, 1), dtype=np.float32)
    convW = ip["conv_W"].reshape(DIN, 9)
    base = dict(projW=ip["proj_W"], projb=col(ip["proj_b"]), W1=_bf(W1),
                b1=col(b1),
                cdiag0=_bf(_diag9(convW[0:DT0], DT0)),
                cdiag1=_bf(_diag9(convW[DT0:DIN], DT1)),
                convb=col(ip["conv_b"]),
                eye=_bf(np.eye(128, dtype=np.float32)))
    # packed dt1 (channels 128:192 as p = d + 64*nh, 8 states per slot)
    cd1 = np.zeros((DT1, 9, 128), np.float32)
    di = np.arange(DT1)
    for j in range(9):
        cd1[di, j, di] = convW[DT0 + di, j]
        cd1[di, j, DT1 + di] = convW[DT0 + di, j]
    base["cdiag1d"] = _bf(cd1)
    base["cbd"] = col(np.tile(ip["conv_b"][DT0:], 2))
    base["red1"] = _bf(np.tile(np.eye(DT1, dtype=np.float32), (2, 1)))
    maps = []
    for c in range(8):
        b, plane = c // 2, c % 2
        ks = [plane, plane + 2]
        m = dict(base)
        m["xc_t"] = np.ascontiguousarray(ip["x_cat"][b].reshape(L, CIN).T)
        m["xpw"] = _bf(np.stack([ip["x_proj_W"][k].T for k in ks]))
        xpz = np.zeros((2, 128, RNK + 2 * NST), np.float32)
        for kk, k in enumerate(ks):
            xpz[kk, 0:DT1] = ip["x_proj_W"][k].T[DT0:DIN]
        m["xpz"] = _bf(xpz)
        m["dtw"] = _bf(np.stack([ip["dt_W"][k].T for k in ks]))
        m["dtwd"] = _bf(np.stack(
            [np.tile(ip["dt_W"][k].T[:, DT0:], (1, 2)) for k in ks]))
        m["dtb"] = np.ascontiguousarray(np.stack([col(ip["dt_b"][k]) for k in ks]))
        m["dtbd"] = np.ascontiguousarray(np.stack(
            [col(np.tile(ip["dt_b"][k][DT0:], 2)) for k in ks]))
        m["acoef"] = np.ascontiguousarray(np.stack([A[k] for k in ks]))
        acp = np.zeros((2, 128, 8), np.float32)
        for kk, k in enumerate(ks):
            for nh in range(2):
                acp[kk, nh * DT1:(nh + 1) * DT1, :] = A[k][DT0:DIN,
                                                           nh * 8:(nh + 1) * 8]
        m["acp"] = np.ascontiguousarray(acp)
        m["dsum"] = col(Ds[ks[0]] + Ds[ks[1]])
        m["mrow"] = np.full((DIN, 1), 1.0 - plane, np.float32)
        m["mcol"] = np.full((DIN, 1), float(plane), np.float32)
        maps.append(m)
    return maps


def prep_pass2(ip, res1):
    OPm = (np.diag(ip["out_norm_w"]) @ ip["out_proj_W"]).astype(np.float32)
    OPB = (np.diag(ip["out_norm_b"]) @ ip["out_proj_W"]).astype(np.float32)
    col = lambda v: np.ascontiguousarray(v.reshape(-1, 1), dtype=np.float32)
    cbw = ip["cb_dw_W"].reshape(HID, 9)
    base = dict(OPm=_bf(OPm), OPB=_bf(OPB),
                PW1=_bf(ip["cb_pw1_W"][:, :, 0, 0].T),
                g1=col(ip["cb_bn1_g"]), bb1=col(ip["cb_bn1_b"]),
                cbdiag0=_bf(_diag9(cbw[0:DT0], DT0)),
                cbdiag1=_bf(_diag9(cbw[DT0:HID], DT1)),
                g2=col(ip["cb_bn2_g"]), bb2=col(ip["cb_bn2_b"]),
                PW2=_bf(ip["cb_pw2_W"][:, :, 0, 0].T),
                g3=col(ip["cb_bn3_g"]), bb3=col(ip["cb_bn3_b"]),
                fw=col(ip["norm_w"]), fb=col(ip["norm_b"]))
    maps = []
    for c in range(8):
        b = c // 2
        m = dict(base)
        ymf = (np.asarray(res1[2 * b]["oq"], np.float32)
               + np.asarray(res1[2 * b + 1]["oq"], np.float32))
        m["ym"] = _bf(ymf)
        m["xin"] = np.asarray(res1[2 * b]["ox"], np.float32)
        m["zin"] = np.ascontiguousarray(res1[2 * b]["oz"])
        maps.append(m)
    return maps


def kernel(**inputs):
    ip = {k: np.asarray(v, np.float32) for k, v in inputs.items()}
    nc1, nc2 = _get_ncs()
    res1 = run_bass_kernel_spmd(nc1, prep_pass1(ip), list(range(8))).results
    res2 = run_bass_kernel_spmd(nc2, prep_pass2(ip, res1), list(range(8))).results
    outs = [np.asarray(res2[2 * b]["o"], np.float32).T.reshape(H_, W_, COUT)
            for b in range(B_)]
    return np.stack(outs).astype(np.float32)


# revision 44
# speedup vs baseline: 2.1216x; 1.0848x over previous
"""Trainium2 Bass kernel for nn_DecoderFusionBlock (VSS/Mamba decoder fusion block).

Two-pass SPMD over 8 cores:
  pass 1: core c -> batch b=c//2, plane=c%2 (row-/col-major spatial order).
          proj/LN/in_proj (f32r / bf16 matmuls), depthwise conv via PE diag
          matmuls, then the selective scan for the plane's two directions.
          bf16 data path with fp32 scan state; B/C broadcast to all channel
          partitions via a DRAM-staged broadcast DMA so the big elementwise
          multiplies run in the DVE 2x (2-byte) mode; the n-state reduction
          runs on the PE as identity-weight matmul accumulation in PSUM.
  host:   ym[b] = Q[2b] + Q[2b+1]  (the only cross-core reduction)
  pass 2: core c -> batch b=c//2: out-norm, gate, out_proj+residual,
          ConvBlock (conv again via PE), final LayerNorm.
"""

import contextlib
import numpy as np

import concourse.bass as bass
import concourse.tile as tile
from concourse import bacc, mybir
from concourse.bass_utils import run_bass_kernel_spmd

f32 = mybir.dt.float32
f32r = mybir.dt.float32r
bf16 = mybir.dt.bfloat16
AF = mybir.ActivationFunctionType
OP_ = mybir.AluOpType

B_, H_, W_ = 4, 48, 48
L = H_ * W_
CIN, COUT = 192, 96
DIN, NST, RNK, KDIR = 192, 16, 6, 4
HID = 192
EPS = 1e-5
DT0, DT1 = 128, 64
MMC = 512
MM = [(s, min(MMC, L - s)) for s in range(0, L, MMC)]
LC = 256
SC = [(i * LC, LC) for i in range(L // LC)]
CROWS = [(0, 10), (10, 10), (20, 10), (30, 10), (40, 8)]


def _rev(ap, s, w):
    hi = L - 1 - s
    lo = hi - w
    return ap[:, hi::-1] if lo < 0 else ap[:, hi:lo:-1]


def _sl(ap, k, s, w):
    return ap[:, s:s + w] if k == 0 else _rev(ap, s, w)


def _rep(a, n):
    return bass.AP(tensor=a.tensor, offset=a.offset, ap=[a.ap[0], [0, n], a.ap[1]])


def _twh(a):
    st = a.ap[1][0]
    return bass.AP(tensor=a.tensor, offset=a.offset,
                   ap=[a.ap[0], [st, 48], [48 * st, 48]])


def _pl3(a):
    st = a.ap[1][0]
    return bass.AP(tensor=a.tensor, offset=a.offset,
                   ap=[a.ap[0], [48 * st, 48], [st, 48]])


# ---------------------------------------------------------------- pass 1
def build_nc1():
    nc = bacc.Bacc("TRN2", target_bir_lowering=False, debug=False, num_devices=8)
    din = {}

    def I(name, shape, dt=f32):
        din[name] = nc.dram_tensor(name, shape, dt, kind="ExternalInput")

    I("xc_t", [CIN, L], bf16)
    I("projW", [CIN, COUT], bf16); I("projb", [COUT, 1])
    I("W1", [COUT, 2 * DIN], bf16); I("b1", [2 * DIN, 1])
    I("cdiag0", [DT0, 9, DT0], bf16); I("cdiag1", [DT1, 9, DT1], bf16)
    I("convb", [DIN, 1])
    I("eye", [128, 128], bf16)
    I("xpw", [2, DIN, RNK + 2 * NST], bf16)
    I("xpz", [2, 128, RNK + 2 * NST], bf16)
    I("dtw", [2, RNK, DIN], bf16); I("dtwd", [2, RNK, 128], bf16)
    I("dtb", [2, DIN, 1]); I("dtbd", [2, 128, 1])
    I("acoef", [2, DIN, NST]); I("acp", [2, 128, 8]); I("dsum", [DIN, 1])
    I("cdiag1d", [DT1, 9, 128], bf16); I("cbd", [128, 1])
    I("red1", [128, DT1], bf16)
    I("mrow", [DIN, 1]); I("mcol", [DIN, 1])
    oq_d = nc.dram_tensor("oq", [DIN, L], bf16, kind="ExternalOutput")
    ox_d = nc.dram_tensor("ox", [COUT, L], f32, kind="ExternalOutput")
    oz_d = nc.dram_tensor("oz", [DIN, L], bf16, kind="ExternalOutput")
    bcd = nc.dram_tensor("BCd", [2, 32, L], bf16, kind="Internal")

    ctx = contextlib.ExitStack()
    with tile.TileContext(nc) as tc, ctx:
        const = ctx.enter_context(tc.tile_pool(name="const", bufs=1))
        big = ctx.enter_context(tc.tile_pool(name="big", bufs=1))
        work = ctx.enter_context(tc.tile_pool(name="work", bufs=2))
        scn = ctx.enter_context(tc.tile_pool(name="scn", bufs=1))
        psM = ctx.enter_context(tc.tile_pool(name="psM", bufs=2, space="PSUM"))
        psY = ctx.enter_context(tc.tile_pool(name="psY", bufs=2, space="PSUM"))

        def load2(name, rows, cols, dt=f32):
            t0 = const.tile([DT0, cols], dt, tag=name + "0", name=name + "0")
            t1 = const.tile([DT1, cols], dt, tag=name + "1", name=name + "1")
            nc.sync.dma_start(t0[:], din[name][0:DT0])
            nc.sync.dma_start(t1[:], din[name][DT0:rows])
            return t0, t1

        # input data first so the proj chain isn't stuck behind const loads
        xc0 = big.tile([DT0, L], bf16, tag="xc0")
        xc1 = big.tile([DT1, L], bf16, tag="xc1")
        nc.sync.dma_start(xc0[:], din["xc_t"][0:DT0])
        nc.sync.dma_start(xc1[:], din["xc_t"][DT0:CIN])
        projW0 = const.tile([DT0, COUT], bf16)
        projW1 = const.tile([DT1, COUT], bf16)
        nc.sync.dma_start(projW0[:], din["projW"][0:DT0])
        nc.sync.dma_start(projW1[:], din["projW"][DT0:CIN])
        projb = const.tile([COUT, 1], f32)
        nc.sync.dma_start(projb[:], din["projb"][:])
        W1t = const.tile([COUT, 2 * DIN], bf16)
        nc.sync.dma_start(W1t[:], din["W1"][:])
        b1x0 = const.tile([DT0, 1], f32); nc.sync.dma_start(b1x0[:], din["b1"][0:128])
        b1x1 = const.tile([DT1, 1], f32); nc.sync.dma_start(b1x1[:], din["b1"][128:192])
        b1z0 = const.tile([DT0, 1], f32); nc.sync.dma_start(b1z0[:], din["b1"][192:320])
        b1z1 = const.tile([DT1, 1], f32); nc.sync.dma_start(b1z1[:], din["b1"][320:384])
        cdiag0 = const.tile([DT0, 9, DT0], bf16)
        nc.sync.dma_start(cdiag0[:], din["cdiag0"][:])
        cdiag1 = const.tile([DT1, 9, DT1], bf16)
        nc.sync.dma_start(cdiag1[:], din["cdiag1"][:])
        convb0, convb1 = load2("convb", DIN, 1)
        cdiag1d = const.tile([DT1, 9, 128], bf16)
        nc.sync.dma_start(cdiag1d[:], din["cdiag1d"][:])
        cbd = const.tile([128, 1], f32)
        nc.sync.dma_start(cbd[:], din["cbd"][:])
        red1 = const.tile([128, DT1], bf16)
        nc.sync.dma_start(red1[:], din["red1"][:])
        eye = const.tile([128, 128], bf16)
        nc.sync.dma_start(eye[:], din["eye"][:])
        dsum0, dsum1 = load2("dsum", DIN, 1)
        mrow0, mrow1 = load2("mrow", DIN, 1)
        mcol0, mcol1 = load2("mcol", DIN, 1)
        kw = []
        for k in range(2):
            xp0 = const.tile([DT0, RNK + 2 * NST], bf16, name=f"xp{k}0")
            xp1 = const.tile([128, RNK + 2 * NST], bf16, name=f"xp{k}1")
            nc.sync.dma_start(xp0[:], din["xpw"][k, 0:DT0])
            nc.sync.dma_start(xp1[:], din["xpz"][k])
            dtw = const.tile([38, DIN], bf16, tag="dtwm", name=f"dtw{k}",
                             bufs=1) if k == 0 else kw[0]["dtwt"]
            nc.sync.dma_start(dtw[k * 32:k * 32 + RNK], din["dtw"][k])
            dtwd = const.tile([38, 128], bf16, tag="dtwdm", name=f"dtwd{k}",
                              bufs=1) if k == 0 else kw[0]["dtwdt"]
            nc.sync.dma_start(dtwd[k * 32:k * 32 + RNK], din["dtwd"][k])
            dtb0 = const.tile([DT0, 1], f32, name=f"dtb{k}0")
            dtb1 = const.tile([128, 1], f32, name=f"dtb{k}1")
            nc.sync.dma_start(dtb0[:], din["dtb"][k, 0:DT0])
            nc.sync.dma_start(dtb1[:], din["dtbd"][k])
            ac0 = const.tile([DT0, NST], f32, name=f"ac{k}0")
            ac1 = const.tile([128, 8], f32, name=f"ac{k}1")
            nc.sync.dma_start(ac0[:], din["acoef"][k, 0:DT0])
            nc.sync.dma_start(ac1[:], din["acp"][k])
            kw.append(dict(xp=(xp0, xp1), dtwt=dtw, dtwdt=dtwd,
                           dtw=dtw[k * 32:k * 32 + RNK],
                           dtwd=dtwd[k * 32:k * 32 + RNK],
                           dtb=(dtb0, dtb1),
                           ac=(ac0, ac1)))

        ones128 = const.tile([128, 1], f32); nc.vector.memset(ones128[:], 1.0)
        onesrow = const.tile([1, 128], bf16); nc.vector.memset(onesrow[:], 1.0)
        epsc = const.tile([1, 1], f32); nc.vector.memset(epsc[:], EPS)

        # ---- proj (f32r matmuls, x_t kept fp32 for residual) ----
        x_t = big.tile([COUT, L], f32, tag="x_t")
        for (s, w) in MM:
            ps = psM.tile([128, MMC], f32, tag="mm", name="psproj")
            nc.tensor.matmul(ps[:COUT, :w], projW0[:], xc0[:, s:s + w],
                             start=True, stop=False)
            nc.tensor.matmul(ps[:COUT, :w], projW1[:], xc1[:, s:s + w],
                             start=False, stop=True)
            nc.scalar.activation(x_t[:, s:s + w], ps[:COUT, :w], AF.Identity,
                                 bias=projb[:])
        nc.sync.dma_start(ox_d[:], x_t[:])

        # ---- LN1 (Copy + Sqrt share the act-table phase) -> xn bf16 ----
        xn_t = big.tile([COUT, L], bf16, tag="xn")
        for (s, w) in MM:
            ps1 = psM.tile([128, MMC], f32, tag="mm", name="pss1")
            nc.tensor.matmul(ps1[:1, :w], ones128[:COUT], x_t[:, s:s + w],
                             start=True, stop=True)
            mrw = work.tile([1, MMC], bf16, tag="mrw", bufs=1)
            nc.scalar.activation(mrw[:, :w], ps1[:1, :w], AF.Copy, scale=1.0 / COUT)
            sq = work.tile([128, MMC], f32, tag="sqc", bufs=1)
            nc.vector.tensor_tensor(out=sq[:COUT, :w], in0=x_t[:, s:s + w],
                                    in1=x_t[:, s:s + w], op=OP_.mult)
            ps2 = psM.tile([128, MMC], f32, tag="mm", name="pss2")
            nc.tensor.matmul(ps2[:1, :w], ones128[:COUT], sq[:COUT, :w],
                             start=True, stop=True)
            mq = work.tile([1, MMC], f32, tag="mq", bufs=1)
            nc.scalar.activation(mq[:, :w], ps2[:1, :w], AF.Copy, scale=1.0 / COUT)
            msq = work.tile([1, MMC], f32, tag="msq", bufs=1)
            nc.vector.tensor_tensor(out=msq[:, :w], in0=mrw[:, :w],
                                    in1=mrw[:, :w], op=OP_.mult)
            nc.vector.tensor_tensor(out=mq[:, :w], in0=mq[:, :w],
                                    in1=msq[:, :w], op=OP_.subtract)
            nc.scalar.activation(mq[:, :w], mq[:, :w], AF.Sqrt, bias=epsc[:])
            rsw = work.tile([1, MMC], bf16, tag="rsw", bufs=1)
            with nc.allow_low_precision(reason="bf16 1/std is well conditioned"):
                nc.vector.reciprocal(rsw[:, :w], mq[:, :w])
            pm = psM.tile([128, MMC], f32, tag="mm", name="psbm")
            nc.tensor.matmul(pm[:, :w], onesrow[:], mrw[:, :w],
                             start=True, stop=True)
            pr = psM.tile([128, MMC], f32, tag="mm", name="psbr")
            nc.tensor.matmul(pr[:, :w], onesrow[:], rsw[:, :w],
                             start=True, stop=True)
            xn_ = work.tile([128, MMC], bf16, tag="xn_", bufs=1)
            nc.vector.tensor_tensor(out=xn_[:COUT, :w], in0=x_t[:, s:s + w],
                                    in1=pm[:COUT, :w], op=OP_.subtract)
            nc.vector.tensor_tensor(out=xn_t[:, s:s + w], in0=xn_[:COUT, :w],
                                    in1=pr[:COUT, :w], op=OP_.mult)

        # ---- in_proj (xm tiles bf16; z silu'd -> DRAM bf16) ----
        xm0 = big.tile([DT0, L], bf16, tag="xm0")
        xm1 = big.tile([DT1, L], bf16, tag="xm1")
        for (s, w) in MM:
            for (coff, rows, bcol, dst, zoff) in (
                    (0, DT0, b1x0, xm0, None), (DT0, DT1, b1x1, xm1, None),
                    (DIN, DT0, b1z0, None, 0), (DIN + DT0, DT1, b1z1, None, DT0)):
                psi = psM.tile([128, MMC], f32, tag="mm", name="psip")
                nc.tensor.matmul(psi[:rows, :w], W1t[:, coff:coff + rows],
                                 xn_t[:, s:s + w], start=True, stop=True)
                if dst is not None:
                    nc.scalar.activation(dst[:, s:s + w], psi[:rows, :w],
                                         AF.Identity, bias=bcol[:])
                else:
                    zc = work.tile([128, MMC], bf16, tag="zc", bufs=1)
                    nc.scalar.activation(zc[:rows, :w], psi[:rows, :w], AF.Silu,
                                         bias=bcol[:])
                    nc.sync.dma_start(oz_d[zoff:zoff + rows, s:s + w], zc[:rows, :w])

        # ---- depthwise conv via PE diag matmuls + fused SiLU; the
        #      transposed copy for the plane transform happens per row-chunk
        xs0 = big.tile([DT0, L], bf16, tag="xs0")
        xs1 = big.tile([128, L], bf16, tag="xs1")
        xt0 = big.tile([DT0, L], bf16, tag="xm0", name="xt0")
        xt1 = big.tile([128, L], bf16, tag="xm1", name="xt1")
        for (src, cd, srows, orows, out, bcol, xtt, teng) in (
                (xm0, cdiag0, DT0, DT0, xs0, convb0, xt0, nc.vector),
                (xm1, cdiag1d, DT1, 128, xs1, cbd, xt1, nc.vector)):
            pad = big.tile([128, 50, 50], bf16, tag="xc0", name="pad")
            nc.vector.memset(pad[:srows], 0.0)
            r0 = 0
            for (s, w) in MM:
                r1 = (s + w) // 48
                nc.vector.tensor_copy(out=pad[:srows, 1 + r0:1 + r1, 1:49],
                                      in_=_pl3(src[:])[:, r0:r1, :])
                r0 = r1
            for (y0, ny) in CROWS:
                pc = psM.tile([128, MMC], f32, tag="mm", name="pscv")
                for j in range(9):
                    dy, dx = divmod(j, 3)
                    view = pad[:srows, y0 + dy:y0 + dy + ny, dx:dx + 48]
                    nc.tensor.matmul(pc[:orows, :ny * 48], cd[:, j], view,
                                     start=(j == 0), stop=(j == 8))
                nc.scalar.activation(out[:, y0 * 48:(y0 + ny) * 48],
                                     pc[:orows, :ny * 48], AF.Silu, bias=bcol[:])
                # xt[p, x, y] = xs[p, y, x] for this y-chunk
                xin = _pl3(out[:])[:, y0:y0 + ny, :]
                xout = bass.AP(tensor=xtt.tensor, offset=xtt[:].offset + y0,
                               ap=[xtt[:].ap[0], [1, ny], [48, 48]])
                teng.tensor_copy(out=xout, in_=xin)
        nc.vector.tensor_scalar_mul(xs0[:], xs0[:], mrow0[:])
        nc.vector.scalar_tensor_tensor(out=xs0[:], in0=xt0[:], scalar=mcol0[:],
                                       in1=xs0[:], op0=OP_.mult, op1=OP_.add)
        nc.vector.tensor_scalar_mul(xs1[:], xs1[:], mrow0[:])
        nc.vector.scalar_tensor_tensor(out=xs1[:], in0=xt1[:], scalar=mcol0[:],
                                       in1=xs1[:], op0=OP_.mult, op1=OP_.add)

        # ---- U96: x_dbl for both k; B/C staged to DRAM bf16 ----
        u96m = big.tile([38, L], bf16, tag="u96r")
        u96rs = (u96m[0:RNK], u96m[32:38])
        for k in range(2):
            W = kw[k]
            rb = k * 32          # rank section base: 0 (k0) / 32 (k1)
            bb = rb + 32
            cb = 64 if k == 0 else 0
            for (s, w) in MM:
                ps = psM.tile([128, MMC], f32, tag="mm", name="psU")
                for (coff, ubase, m) in ((0, rb, RNK), (RNK, bb, NST),
                                         (RNK + NST, cb, NST)):
                    nc.tensor.matmul(ps[ubase:ubase + m, :w],
                                     W["xp"][0][:, coff:coff + m],
                                     xs0[:, s:s + w], start=True, stop=False)
                    nc.tensor.matmul(ps[ubase:ubase + m, :w],
                                     W["xp"][1][:, coff:coff + m],
                                     xs1[:, s:s + w], start=False, stop=True)
                nc.scalar.activation(u96rs[k][:, s:s + w],
                                     ps[rb:rb + RNK, :w], AF.Copy)
                bcsw = work.tile([112, MMC], bf16, tag="bcsw")
                nc.scalar.activation(bcsw[bb:bb + NST, :w], ps[bb:bb + NST, :w],
                                     AF.Copy)
                nc.scalar.activation(bcsw[cb:cb + NST, :w], ps[cb:cb + NST, :w],
                                     AF.Copy)
                nc.sync.dma_start(bcd[k, 0:NST, s:s + w], bcsw[bb:bb + NST, :w])
                nc.sync.dma_start(bcd[k, NST:32, s:s + w], bcsw[cb:cb + NST, :w])

        # ---- delta (Softplus) for both k, both dt ----
        dcs = []
        for k in range(2):
            W = kw[k]
            dc0 = big.tile([DT0, L], bf16, tag="xc0" if k == 0 else "xc1",
                           name=f"dc{k}0")
            dc1 = big.tile([128, L], bf16, tag=f"dc{k}1", name=f"dc{k}1")
            for (s, w) in MM:
                for dt, (rows, stat, dct) in enumerate(
                        ((DT0, W["dtw"][:, 0:DT0], dc0),
                         (128, W["dtwd"], dc1))):
                    ps = psM.tile([128, MMC], f32, tag="mm", name="psdt")
                    nc.tensor.matmul(ps[:rows, :w], stat,
                                     u96rs[k][:, s:s + w],
                                     start=True, stop=True)
                    nc.scalar.activation(dct[:, s:s + w], ps[:rows, :w],
                                         AF.Exp, bias=W["dtb"][dt][:])
            # in-place Ln (softplus) + dxc = delta*x, per chunk
            dx0 = big.tile([DT0, L], bf16, tag="x_t" if k == 0 else "xn",
                           name=f"dx{k}0")
            dx1 = big.tile([128, L], bf16, tag="xm1" if k == 0 else "u96r",
                           name=f"dx{k}1")
            for (s, w) in MM:
                for dct, dxt, xst in ((dc0, dx0, xs0), (dc1, dx1, xs1)):
                    nc.scalar.activation(dct[:, s:s + w], dct[:, s:s + w],
                                         AF.Ln, bias=1.0)
                    nc.vector.tensor_tensor(out=dxt[:, s:s + w],
                                            in0=dct[:, s:s + w],
                                            in1=xst[:, s:s + w], op=OP_.mult)
            dcs.append((dc0, dc1, dx0, dx1))

        # ---- P init with the direction-independent D term ----
        P0 = big.tile([DT0, L], bf16, tag="xm0", name="P0")
        P1 = big.tile([DT1, L], bf16, tag="xt1m", name="P1")
        nc.vector.tensor_scalar_mul(P0[:], xs0[:], dsum0[:])
        nc.gpsimd.tensor_scalar_mul(P1[:], xs1[0:DT1], dsum1[:])

        # ---- selective scan ----
        hp = [[scn.tile([128, NST if dt == 0 else 8, 1], bf16,
                        tag=f"hp{k}{dt}", name=f"hp{k}{dt}")
               for dt in range(2)] for k in range(2)]
        for k in range(2):
            for dt in range(2):
                nc.vector.memset(hp[k][dt][:], 0.0)

        pairs = [(c, k) for c in range(len(SC)) for k in range(2)]
        bcr_tiles = {}

        def issue_bcast(i):
            c, k = pairs[i]
            s, w = SC[c]
            s0 = s if k == 0 else L - s - w
            t = scn.tile([128, 40, LC], bf16, tag="bcr", bufs=3,
                         name=f"bcr{i}")
            src = bass.AP(tensor=bcd, offset=k * 32 * L + s0,
                          ap=[[0, 128], [L, 32], [1, w]])
            nc.sync.dma_start(t[:, 0:32], src)
            # packed dt1 C': rows 0:64 get states 16:24, rows 64:128 get 24:32
            for (p0, srow) in ((0, 16), (64, 24)):
                sp = bass.AP(tensor=bcd, offset=k * 32 * L + srow * L + s0,
                             ap=[[0, 64], [L, 8], [1, w]])
                nc.sync.dma_start(t[p0:p0 + 64, 32:40, :], sp)
            bcr_tiles[i] = t

        pendingP = []

        def tail(dt, dA, Ht, crep, hpt, pdst):
            # after the scan: save carry, G = H*C, PE n-reduction; the P
            # accumulate is deferred further so DVE never waits on the PE
            if len(pendingP) >= 2:
                pendingP.pop(0)()
            nc.gpsimd.tensor_copy(out=hpt[:], in_=Ht[:, :, LC:LC + 1])
            psy = psY.tile([128, LC], f32, tag="psy", name="psy", bufs=3)
            if dt == 0:
                nc.vector.tensor_tensor(out=dA[:, :, 1:], in0=Ht[:, :, 1:],
                                        in1=crep, op=OP_.mult)
                for n in range(NST):
                    nc.tensor.matmul(psy[:, :], eye[:, :],
                                     dA[:, n, 1:],
                                     start=(n == 0), stop=(n == NST - 1))
                rows = DT0
            else:
                nc.vector.tensor_tensor(out=dA[:, :, 1:],
                                        in0=Ht[:, :, 1:],
                                        in1=crep, op=OP_.mult)
                for n in range(8):
                    nc.tensor.matmul(psy[:DT1, :], red1[:, :],
                                     dA[:, n, 1:],
                                     start=(n == 0), stop=(n == 7))
                rows = DT1

            def paccum(psy=psy, rows=rows, pdst=pdst):
                nc.vector.tensor_tensor(out=pdst, in0=psy[:rows, :], in1=pdst,
                                        op=OP_.add)
            pendingP.append(paccum)

        pending = []
        issue_bcast(0)
        for i, (c, k) in enumerate(pairs):
            s, w = SC[c]
            W = kw[k]
            bcr = bcr_tiles.pop(i)
            if k == 0:
                brep = bcr[:, 0:NST, :]
                crep = bcr[:, NST:32, :]
            else:
                brep = bcr[:, 0:NST, ::-1]
                crep = bcr[:, NST:32, ::-1]
            for dt in range(2):
                ns = NST if dt == 0 else 8
                if len(pending) >= 3:
                    # flush the 3-iterations-old deferred tail before its
                    # dA/dBu/Ht buffer slots are reused below
                    pending.pop(0)()
                if dt == 0 and i + 1 < len(pairs):
                    # safe point: all readers of bcr slot (i+1)%3's previous
                    # occupant (pair i-2) have been emitted by now
                    issue_bcast(i + 1)
                dct = dcs[k][dt]
                dcsl = _sl(dct[:], k, s, w)
                dxsl = _sl(dcs[k][2 + dt][:], k, s, w)
                dA = scn.tile([128, ns, LC + 1], bf16, tag="dA",
                              name=f"dA{dt}", bufs=3)
                dBu = scn.tile([128, ns, LC + 1], bf16, tag="dBu",
                               name=f"dBu{dt}", bufs=2)
                Ht = scn.tile([128, ns, LC + 1], bf16, tag="Ht",
                              name=f"Ht{dt}", bufs=3)
                nc.gpsimd.memset(dA[:, :, 0:1], 0.0)
                nc.gpsimd.tensor_copy(out=dBu[:, :, 0:1], in_=hp[k][dt][:])
                for n in range(ns):
                    nc.scalar.activation(dA[:, n, 1:], dcsl, AF.Exp,
                                         scale=W["ac"][dt][:, n:n + 1])
                beng = nc.vector if dt == 0 else nc.gpsimd
                if dt == 0:
                    beng.tensor_tensor(out=dBu[:, :, 1:],
                                       in0=_rep(dxsl, NST),
                                       in1=brep, op=OP_.mult)
                else:
                    # packed: nh half selects B rows 0:8 / 8:16
                    beng.tensor_tensor(out=dBu[0:DT1, :, 1:],
                                       in0=_rep(dxsl[0:DT1], 8),
                                       in1=brep[0:DT1, 0:8], op=OP_.mult)
                    beng.tensor_tensor(out=dBu[DT1:128, :, 1:],
                                       in0=_rep(dxsl[DT1:128], 8),
                                       in1=brep[DT1:128, 8:NST], op=OP_.mult)
                nc.vector.tensor_tensor_scan(
                    out=Ht[:].rearrange("p a b -> p (a b)"),
                    data0=dA[:].rearrange("p a b -> p (a b)"),
                    data1=dBu[:].rearrange("p a b -> p (a b)"),
                    initial=0.0, op0=OP_.mult, op1=OP_.add)
                Pt = P0 if dt == 0 else P1
                cr = crep if dt == 0 else (
                    bcr[:, 32:40, :] if k == 0 else bcr[:, 32:40, ::-1])
                args = (dt, dA, Ht, cr, hp[k][dt], _sl(Pt[:], k, s, w))
                pending.append(lambda a=args: tail(*a))
        for fn in pending:
            fn()
        for fn in pendingP:
            fn()

        # ---- Q = mrow*P + mcol*transpose(P) ----
        Q0 = big.tile([DT0, L], bf16, tag="xs0", name="Q0")
        Q1 = big.tile([DT1, L], bf16, tag="xs1", name="Q1")
        nc.vector.tensor_scalar_mul(Q0[:], _twh(P0[:]), mcol0[:])
        nc.vector.scalar_tensor_tensor(out=Q0[:], in0=P0[:], scalar=mrow0[:],
                                       in1=Q0[:], op0=OP_.mult, op1=OP_.add)
        nc.gpsimd.tensor_scalar_mul(Q1[:], _twh(P1[:]), mcol1[:])
        qtm = big.tile([DT1, L], bf16, tag="xm1", name="qtm")
        nc.gpsimd.tensor_scalar_mul(qtm[:], P1[:], mrow1[:])
        nc.gpsimd.tensor_tensor(out=Q1[:], in0=qtm[:], in1=Q1[:], op=OP_.add)
        nc.sync.dma_start(oq_d[0:DT0], Q0[:])
        nc.sync.dma_start(oq_d[DT0:DIN], Q1[:])
    nc.compile()
    return nc


# ---------------------------------------------------------------- pass 2
def build_nc2():
    nc = bacc.Bacc("TRN2", target_bir_lowering=False, debug=False, num_devices=8)
    din = {}

    def I(name, shape, dt=f32):
        din[name] = nc.dram_tensor(name, shape, dt, kind="ExternalInput")

    I("ym", [DIN, L], bf16); I("xin", [COUT, L]); I("zin", [DIN, L], bf16)
    I("OPm", [DIN, COUT], bf16); I("OPB", [DIN, COUT], bf16)
    I("PW1", [COUT, HID], bf16); I("g1", [HID, 1]); I("bb1", [HID, 1])
    I("cbdiag0", [DT0, 9, DT0], bf16); I("cbdiag1", [DT1, 9, DT1], bf16)
    I("g2", [HID, 1]); I("bb2", [HID, 1])
    I("PW2", [HID, COUT], bf16); I("g3", [COUT, 1]); I("bb3", [COUT, 1])
    I("fw", [COUT, 1]); I("fb", [COUT, 1])
    out_d = nc.dram_tensor("o", [COUT, L], f32, kind="ExternalOutput")

    ctx = contextlib.ExitStack()
    with tile.TileContext(nc) as tc, ctx:
        const = ctx.enter_context(tc.tile_pool(name="const", bufs=1))
        big = ctx.enter_context(tc.tile_pool(name="big", bufs=1))
        work = ctx.enter_context(tc.tile_pool(name="work", bufs=2))
        psM = ctx.enter_context(tc.tile_pool(name="psM", bufs=2, space="PSUM"))

        def load2(name, rows, cols, dt=f32):
            t0 = const.tile([DT0, cols], dt, tag=name + "0", name=name + "0")
            t1 = const.tile([DT1, cols], dt, tag=name + "1", name=name + "1")
            nc.sync.dma_start(t0[:], din[name][0:DT0])
            nc.sync.dma_start(t1[:], din[name][DT0:rows])
            return t0, t1

        def load1(name, rows):
            t = const.tile([rows, 1], f32, tag=name, name=name)
            nc.sync.dma_start(t[:], din[name][:])
            return t

        # input data first so the out-norm chain isn't stuck behind consts
        ym0 = big.tile([DT0, L], bf16, tag="ym0")
        ym1 = big.tile([DT1, L], bf16, tag="ym1")
        nc.sync.dma_start(ym0[:], din["ym"][0:DT0])
        nc.sync.dma_start(ym1[:], din["ym"][DT0:DIN])
        xres = big.tile([COUT, L], f32, tag="xres")
        nc.sync.dma_start(xres[:], din["xin"][:])
        zc0 = big.tile([DT0, L], bf16, tag="zc0")
        zc1 = big.tile([DT1, L], bf16, tag="zc1")
        nc.sync.dma_start(zc0[:], din["zin"][0:DT0])
        nc.sync.dma_start(zc1[:], din["zin"][DT0:DIN])
        OP0, OP1 = load2("OPm", DIN, COUT, bf16)
        OPB0, OPB1 = load2("OPB", DIN, COUT, bf16)
        PW1t = const.tile([COUT, HID], bf16)
        nc.sync.dma_start(PW1t[:], din["PW1"][:])
        g1c0, g1c1 = load2("g1", HID, 1)
        bb1c0, bb1c1 = load2("bb1", HID, 1)
        cbd0 = const.tile([DT0, 9, DT0], bf16)
        nc.sync.dma_start(cbd0[:], din["cbdiag0"][:])
        cbd1 = const.tile([DT1, 9, DT1], bf16)
        nc.sync.dma_start(cbd1[:], din["cbdiag1"][:])
        g2c0, g2c1 = load2("g2", HID, 1)
        bb2c0, bb2c1 = load2("bb2", HID, 1)
        PW20, PW21 = load2("PW2", HID, COUT, bf16)
        g3c = load1("g3", COUT); bb3c = load1("bb3", COUT)
        fwc = load1("fw", COUT); fbc = load1("fb", COUT)
        onesb = const.tile([128, 1], bf16); nc.vector.memset(onesb[:], 1.0)
        onesrow = const.tile([1, 128], bf16); nc.vector.memset(onesrow[:], 1.0)
        epsc = const.tile([1, 1], f32); nc.vector.memset(epsc[:], EPS)

        # out-norm stats over 192 partitions (per-chunk)
        mean_r = big.tile([1, L], bf16, tag="mean")
        rs_r = big.tile([1, L], bf16, tag="rs")
        for (s, w) in MM:
            ps = psM.tile([128, MMC], f32, tag="mm", name="pso1")
            nc.tensor.matmul(ps[:1, :w], onesb[:], ym0[:, s:s + w],
                             start=True, stop=False)
            nc.tensor.matmul(ps[:1, :w], onesb[:DT1], ym1[:, s:s + w],
                             start=False, stop=True)
            nc.scalar.activation(mean_r[:, s:s + w], ps[:1, :w], AF.Copy,
                                 scale=1.0 / DIN)
            ps2 = psM.tile([128, MMC], f32, tag="mm", name="pso2")
            for i, (t, rows) in enumerate(((ym0, DT0), (ym1, DT1))):
                sq = work.tile([128, MMC], bf16, tag="sqc", bufs=1)
                nc.vector.tensor_tensor(out=sq[:rows, :w], in0=t[:, s:s + w],
                                        in1=t[:, s:s + w], op=OP_.mult)
                nc.tensor.matmul(ps2[:1, :w], onesb[:rows], sq[:rows, :w],
                                 start=(i == 0), stop=(i == 1))
            mq = work.tile([1, MMC], f32, tag="mq", bufs=1)
            nc.scalar.activation(mq[:, :w], ps2[:1, :w], AF.Copy,
                                 scale=1.0 / DIN)
            msqc = work.tile([1, MMC], f32, tag="msqc", bufs=1)
            nc.vector.tensor_tensor(out=msqc[:, :w], in0=mean_r[:, s:s + w],
                                    in1=mean_r[:, s:s + w], op=OP_.mult)
            nc.vector.tensor_tensor(out=mq[:, :w], in0=mq[:, :w],
                                    in1=msqc[:, :w], op=OP_.subtract)
            nc.scalar.activation(mq[:, :w], mq[:, :w], AF.Sqrt, bias=epsc[:])
            with nc.allow_low_precision(reason="bf16 1/std is well conditioned"):
                nc.vector.reciprocal(rs_r[:, s:s + w], mq[:, :w])

        x2f = big.tile([COUT, L], f32, tag="x2f")
        x2b = big.tile([COUT, L], bf16, tag="x2b")
        for (s, w) in MM:
            pm = psM.tile([128, MMC], f32, tag="mm", name="psm")
            nc.tensor.matmul(pm[:, :w], onesrow[:], mean_r[:, s:s + w],
                             start=True, stop=True)
            pr = psM.tile([128, MMC], f32, tag="mm", name="psr")
            nc.tensor.matmul(pr[:, :w], onesrow[:], rs_r[:, s:s + w],
                             start=True, stop=True)
            po = psM.tile([128, MMC], f32, tag="mm", name="pso")
            for i, (t, z, rows) in enumerate(((ym0, zc0, DT0), (ym1, zc1, DT1))):
                yn = work.tile([128, MMC], bf16, tag=f"yn{i}", name=f"yn{i}")
                nc.vector.tensor_tensor(out=yn[:rows, :w], in0=t[:, s:s + w],
                                        in1=pm[:rows, :w], op=OP_.subtract)
                nc.vector.tensor_tensor(out=yn[:rows, :w], in0=yn[:rows, :w],
                                        in1=pr[:rows, :w], op=OP_.mult)
                nc.vector.tensor_tensor(out=yn[:rows, :w], in0=yn[:rows, :w],
                                        in1=z[:, s:s + w], op=OP_.mult)
                OPt = OP0 if i == 0 else OP1
                OPBt = OPB0 if i == 0 else OPB1
                nc.tensor.matmul(po[:COUT, :w], OPt[:], yn[:rows, :w],
                                 start=(i == 0), stop=False)
                nc.tensor.matmul(po[:COUT, :w], OPBt[:], z[:, s:s + w],
                                 start=False, stop=(i == 1))
            nc.vector.tensor_tensor(out=x2f[:, s:s + w], in0=po[:COUT, :w],
                                    in1=xres[:, s:s + w], op=OP_.add)
            nc.scalar.activation(x2b[:, s:s + w], x2f[:, s:s + w], AF.Copy)

        # ConvBlock: PW1 + gelu
        t0 = big.tile([DT0, L], bf16, tag="ym0", name="t0")
        t1 = big.tile([DT1, L], bf16, tag="ym1", name="t1")
        for (s, w) in MM:
            for (dst, coff, rows, gc_, bc_) in ((t0, 0, DT0, g1c0, bb1c0),
                                                (t1, DT0, DT1, g1c1, bb1c1)):
                ps = psM.tile([128, MMC], f32, tag="mm", name="psp1")
                nc.tensor.matmul(ps[:rows, :w], PW1t[:, coff:coff + rows],
                                 x2b[:, s:s + w], start=True, stop=True)
                nc.scalar.activation(dst[:, s:s + w], ps[:rows, :w], AF.Gelu,
                                     bias=bc_[:], scale=gc_[:])
        # dw conv via PE; fused bn2+gelu on psum
        v0 = big.tile([DT0, L], bf16, tag="zc0", name="v0")
        v1 = big.tile([DT1, L], bf16, tag="zc1", name="v1")
        for (src, cd, rows, out, gc_, bc_) in (
                (t0, cbd0, DT0, v0, g2c0, bb2c0),
                (t1, cbd1, DT1, v1, g2c1, bb2c1)):
            pad = work.tile([128, 50, 50], bf16, tag="pad", bufs=1)
            nc.vector.memset(pad[:rows], 0.0)
            r0 = 0
            for (s, w) in MM:
                r1 = (s + w) // 48
                nc.vector.tensor_copy(out=pad[:rows, 1 + r0:1 + r1, 1:49],
                                      in_=_pl3(src[:])[:, r0:r1, :])
                r0 = r1
            for (y0, ny) in CROWS:
                pc = psM.tile([128, MMC], f32, tag="mm", name="pscv")
                for j in range(9):
                    dy, dx = divmod(j, 3)
                    view = pad[:rows, y0 + dy:y0 + dy + ny, dx:dx + 48]
                    nc.tensor.matmul(pc[:rows, :ny * 48], cd[:, j], view,
                                     start=(j == 0), stop=(j == 8))
                nc.scalar.activation(out[:, y0 * 48:(y0 + ny) * 48],
                                     pc[:rows, :ny * 48], AF.Gelu,
                                     bias=bc_[:], scale=gc_[:])
        # PW2 + bn3 + residual
        x3f = big.tile([COUT, L], f32, tag="x3f")
        x3b = big.tile([COUT, L], bf16, tag="xres", name="x3b")
        for (s, w) in MM:
            ps = psM.tile([128, MMC], f32, tag="mm", name="psp2")
            nc.tensor.matmul(ps[:COUT, :w], PW20[:], v0[:, s:s + w],
                             start=True, stop=False)
            nc.tensor.matmul(ps[:COUT, :w], PW21[:], v1[:, s:s + w],
                             start=False, stop=True)
            cbt = work.tile([128, MMC], bf16, tag="cbt", bufs=1)
            nc.scalar.activation(cbt[:COUT, :w], ps[:COUT, :w], AF.Identity,
                                 bias=bb3c[:], scale=g3c[:])
            nc.vector.tensor_tensor(out=x3f[:, s:s + w], in0=cbt[:COUT, :w],
                                    in1=x2f[:, s:s + w], op=OP_.add)
            nc.scalar.activation(x3b[:, s:s + w], x3f[:, s:s + w], AF.Copy)

        # final LN
        mean2 = big.tile([1, L], bf16, tag="mean2")
        rs2 = big.tile([1, L], bf16, tag="rs2")
        for (s, w) in MM:
            ps = psM.tile([128, MMC], f32, tag="mm", name="psf1")
            nc.tensor.matmul(ps[:1, :w], onesb[:COUT], x3b[:, s:s + w],
                             start=True, stop=True)
            nc.scalar.activation(mean2[:, s:s + w], ps[:1, :w], AF.Copy,
                                 scale=1.0 / COUT)
            sq = work.tile([128, MMC], bf16, tag="sqc", bufs=1)
            nc.vector.tensor_tensor(out=sq[:COUT, :w], in0=x3b[:, s:s + w],
                                    in1=x3b[:, s:s + w], op=OP_.mult)
            ps2 = psM.tile([128, MMC], f32, tag="mm", name="psf2")
            nc.tensor.matmul(ps2[:1, :w], onesb[:COUT], sq[:COUT, :w],
                             start=True, stop=True)
            mq2 = work.tile([1, MMC], f32, tag="mq2", bufs=1)
            nc.scalar.activation(mq2[:, :w], ps2[:1, :w], AF.Copy,
                                 scale=1.0 / COUT)
            msqc2 = work.tile([1, MMC], f32, tag="msqc2", bufs=1)
            nc.vector.tensor_tensor(out=msqc2[:, :w], in0=mean2[:, s:s + w],
                                    in1=mean2[:, s:s + w], op=OP_.mult)
            nc.vector.tensor_tensor(out=mq2[:, :w], in0=mq2[:, :w],
                                    in1=msqc2[:, :w], op=OP_.subtract)
            nc.scalar.activation(mq2[:, :w], mq2[:, :w], AF.Sqrt, bias=epsc[:])
            with nc.allow_low_precision(reason="bf16 1/std is well conditioned"):
                nc.vector.reciprocal(rs2[:, s:s + w], mq2[:, :w])
        for (s, w) in MM:
            pm = psM.tile([128, MMC], f32, tag="mm", name="psfm")
            nc.tensor.matmul(pm[:, :w], onesrow[:], mean2[:, s:s + w],
                             start=True, stop=True)
            pr = psM.tile([128, MMC], f32, tag="mm", name="psfr")
            nc.tensor.matmul(pr[:, :w], onesrow[:], rs2[:, s:s + w],
                             start=True, stop=True)
            oc = work.tile([128, MMC], f32, tag="oc", bufs=1)
            nc.vector.tensor_tensor(out=oc[:COUT, :w], in0=x3f[:, s:s + w],
                                    in1=pm[:COUT, :w], op=OP_.subtract)
            nc.vector.tensor_tensor(out=oc[:COUT, :w], in0=oc[:COUT, :w],
                                    in1=pr[:COUT, :w], op=OP_.mult)
            nc.vector.tensor_scalar(out=oc[:COUT, :w], in0=oc[:COUT, :w],
                                    scalar1=fwc[:], scalar2=fbc[:],
                                    op0=OP_.mult, op1=OP_.add)
            nc.sync.dma_start(out_d[:, s:s + w], oc[:COUT, :w])
    nc.compile()
    return nc


_NC1, _NC2 = None, None


def _get_ncs():
    global _NC1, _NC2
    if _NC1 is None:
        _NC1 = build_nc1()
        _NC2 = build_nc2()
    return _NC1, _NC2


def _bf(a):
    import jax.numpy as jnp
    return np.asarray(jnp.asarray(np.asarray(a, np.float32), jnp.bfloat16))


def _diag9(wmat, rows):
    out = np.zeros((rows, 9, rows), np.float32)
    idx = np.arange(rows)
    for j in range(9):
        out[idx, j, idx] = wmat[:, j]
    return out


def prep_pass1(ip):
    W1 = (np.diag(ip["ln1_w"]) @ ip["in_proj_W"]).astype(np.float32)
    b1 = (ip["ln1_b"] @ ip["in_proj_W"] + ip["in_proj_b"]).astype(np.float32)
    A = (-np.exp(ip["A_logs"].astype(np.float64))).astype(np.float32).reshape(KDIR, DIN, NST)
    Ds = ip["Ds"].reshape(KDIR, DIN)
    col = lambda v: np.ascontiguousarray(v.reshape(-1, 1), dtype=np.float32)
    convW = ip["conv_W"].reshape(DIN, 9)
    base = dict(projW=_bf(ip["proj_W"]), projb=col(ip["proj_b"]), W1=_bf(W1),
                b1=col(b1),
                cdiag0=_bf(_diag9(convW[0:DT0], DT0)),
                cdiag1=_bf(_diag9(convW[DT0:DIN], DT1)),
                convb=col(ip["conv_b"]),
                eye=_bf(np.eye(128, dtype=np.float32)))
    # packed dt1 (channels 128:192 as p = d + 64*nh, 8 states per slot)
    cd1 = np.zeros((DT1, 9, 128), np.float32)
    di = np.arange(DT1)
    for j in range(9):
        cd1[di, j, di] = convW[DT0 + di, j]
        cd1[di, j, DT1 + di] = convW[DT0 + di, j]
    base["cdiag1d"] = _bf(cd1)
    base["cbd"] = col(np.tile(ip["conv_b"][DT0:], 2))
    base["red1"] = _bf(np.tile(np.eye(DT1, dtype=np.float32), (2, 1)))
    maps = []
    for c in range(8):
        b, plane = c // 2, c % 2
        ks = [plane, plane + 2]
        m = dict(base)
        m["xc_t"] = _bf(np.ascontiguousarray(ip["x_cat"][b].reshape(L, CIN).T))
        m["xpw"] = _bf(np.stack([ip["x_proj_W"][k].T for k in ks]))
        xpz = np.zeros((2, 128, RNK + 2 * NST), np.float32)
        for kk, k in enumerate(ks):
            xpz[kk, 0:DT1] = ip["x_proj_W"][k].T[DT0:DIN]
        m["xpz"] = _bf(xpz)
        m["dtw"] = _bf(np.stack([ip["dt_W"][k].T for k in ks]))
        m["dtwd"] = _bf(np.stack(
            [np.tile(ip["dt_W"][k].T[:, DT0:], (1, 2)) for k in ks]))
        m["dtb"] = np.ascontiguousarray(np.stack([col(ip["dt_b"][k]) for k in ks]))
        m["dtbd"] = np.ascontiguousarray(np.stack(
            [col(np.tile(ip["dt_b"][k][DT0:], 2)) for k in ks]))
        m["acoef"] = np.ascontiguousarray(np.stack([A[k] for k in ks]))
        acp = np.zeros((2, 128, 8), np.float32)
        for kk, k in enumerate(ks):
            for nh in range(2):
                acp[kk, nh * DT1:(nh + 1) * DT1, :] = A[k][DT0:DIN,
                                                           nh * 8:(nh + 1) * 8]
        m["acp"] = np.ascontiguousarray(acp)
        m["dsum"] = col(Ds[ks[0]] + Ds[ks[1]])
        m["mrow"] = np.full((DIN, 1), 1.0 - plane, np.float32)
        m["mcol"] = np.full((DIN, 1), float(plane), np.float32)
        maps.append(m)
    return maps


def prep_pass2(ip, res1):
    OPm = (np.diag(ip["out_norm_w"]) @ ip["out_proj_W"]).astype(np.float32)
    OPB = (np.diag(ip["out_norm_b"]) @ ip["out_proj_W"]).astype(np.float32)
    col = lambda v: np.ascontiguousarray(v.reshape(-1, 1), dtype=np.float32)
    cbw = ip["cb_dw_W"].reshape(HID, 9)
    base = dict(OPm=_bf(OPm), OPB=_bf(OPB),
                PW1=_bf(ip["cb_pw1_W"][:, :, 0, 0].T),
                g1=col(ip["cb_bn1_g"]), bb1=col(ip["cb_bn1_b"]),
                cbdiag0=_bf(_diag9(cbw[0:DT0], DT0)),
                cbdiag1=_bf(_diag9(cbw[DT0:HID], DT1)),
                g2=col(ip["cb_bn2_g"]), bb2=col(ip["cb_bn2_b"]),
                PW2=_bf(ip["cb_pw2_W"][:, :, 0, 0].T),
                g3=col(ip["cb_bn3_g"]), bb3=col(ip["cb_bn3_b"]),
                fw=col(ip["norm_w"]), fb=col(ip["norm_b"]))
    maps = []
    for c in range(8):
        b = c // 2
        m = dict(base)
        ymf = (np.asarray(res1[2 * b]["oq"], np.float32)
               + np.asarray(res1[2 * b + 1]["oq"], np.float32))
        m["ym"] = _bf(ymf)
        m["xin"] = np.asarray(res1[2 * b]["ox"], np.float32)
        m["zin"] = np.ascontiguousarray(res1[2 * b]["oz"])
        maps.append(m)
    return maps


def kernel(**inputs):
    ip = {k: np.asarray(v, np.float32) for k, v in inputs.items()}
    nc1, nc2 = _get_ncs()
    res1 = run_bass_kernel_spmd(nc1, prep_pass1(ip), list(range(8))).results
    res2 = run_bass_kernel_spmd(nc2, prep_pass2(ip, res1), list(range(8))).results
    outs = [np.asarray(res2[2 * b]["o"], np.float32).T.reshape(H_, W_, COUT)
            for b in range(B_)]
    return np.stack(outs).astype(np.float32)


# revision 46
# speedup vs baseline: 2.1380x; 1.0077x over previous
"""Trainium2 Bass kernel for nn_DecoderFusionBlock (VSS/Mamba decoder fusion block).

Two-pass SPMD over 8 cores:
  pass 1: core c -> batch b=c//2, plane=c%2 (row-/col-major spatial order).
          proj/LN/in_proj (f32r / bf16 matmuls), depthwise conv via PE diag
          matmuls, then the selective scan for the plane's two directions.
          bf16 data path with fp32 scan state; B/C broadcast to all channel
          partitions via a DRAM-staged broadcast DMA so the big elementwise
          multiplies run in the DVE 2x (2-byte) mode; the n-state reduction
          runs on the PE as identity-weight matmul accumulation in PSUM.
  host:   ym[b] = Q[2b] + Q[2b+1]  (the only cross-core reduction)
  pass 2: core c -> batch b=c//2: out-norm, gate, out_proj+residual,
          ConvBlock (conv again via PE), final LayerNorm.
"""

import contextlib
import numpy as np

import concourse.bass as bass
import concourse.tile as tile
from concourse import bacc, mybir
from concourse.bass_utils import run_bass_kernel_spmd

f32 = mybir.dt.float32
f32r = mybir.dt.float32r
bf16 = mybir.dt.bfloat16
AF = mybir.ActivationFunctionType
OP_ = mybir.AluOpType

B_, H_, W_ = 4, 48, 48
L = H_ * W_
CIN, COUT = 192, 96
DIN, NST, RNK, KDIR = 192, 16, 6, 4
HID = 192
EPS = 1e-5
DT0, DT1 = 128, 64
MMC = 512
MM = [(s, min(MMC, L - s)) for s in range(0, L, MMC)]
LC = 256
SC = [(i * LC, LC) for i in range(L // LC)]
CROWS = [(0, 10), (10, 10), (20, 10), (30, 10), (40, 8)]


def _rev(ap, s, w):
    hi = L - 1 - s
    lo = hi - w
    return ap[:, hi::-1] if lo < 0 else ap[:, hi:lo:-1]


def _sl(ap, k, s, w):
    return ap[:, s:s + w] if k == 0 else _rev(ap, s, w)


def _rep(a, n):
    return bass.AP(tensor=a.tensor, offset=a.offset, ap=[a.ap[0], [0, n], a.ap[1]])


def _twh(a):
    st = a.ap[1][0]
    return bass.AP(tensor=a.tensor, offset=a.offset,
                   ap=[a.ap[0], [st, 48], [48 * st, 48]])


def _pl3(a):
    st = a.ap[1][0]
    return bass.AP(tensor=a.tensor, offset=a.offset,
                   ap=[a.ap[0], [48 * st, 48], [st, 48]])


# ---------------------------------------------------------------- pass 1
def build_nc1():
    nc = bacc.Bacc("TRN2", target_bir_lowering=False, debug=False, num_devices=8)
    din = {}

    def I(name, shape, dt=f32):
        din[name] = nc.dram_tensor(name, shape, dt, kind="ExternalInput")

    I("xc_t", [CIN, L], bf16)
    I("projW", [CIN, COUT], bf16); I("projb", [COUT, 1])
    I("W1", [COUT, 2 * DIN], bf16); I("b1", [2 * DIN, 1])
    I("cdiag0", [DT0, 9, DT0], bf16); I("cdiag1", [DT1, 9, DT1], bf16)
    I("convb", [DIN, 1])
    I("eye", [128, 128], bf16)
    I("xpw", [2, DIN, RNK + 2 * NST], bf16)
    I("xpz", [2, 128, RNK + 2 * NST], bf16)
    I("dtw", [2, RNK, DIN], bf16); I("dtwd", [2, RNK, 128], bf16)
    I("dtb", [2, DIN, 1]); I("dtbd", [2, 128, 1])
    I("acoef", [2, DIN, NST]); I("acp", [2, 128, 8]); I("dsum", [DIN, 1])
    I("cdiag1d", [DT1, 9, 128], bf16); I("cbd", [128, 1])
    I("red1", [128, DT1], bf16)
    I("mrow", [DIN, 1]); I("mcol", [DIN, 1])
    oq_d = nc.dram_tensor("oq", [DIN, L], bf16, kind="ExternalOutput")
    ox_d = nc.dram_tensor("ox", [COUT, L], f32, kind="ExternalOutput")
    oz_d = nc.dram_tensor("oz", [DIN, L], bf16, kind="ExternalOutput")
    bcd = nc.dram_tensor("BCd", [2, 32, L], bf16, kind="Internal")

    ctx = contextlib.ExitStack()
    with tile.TileContext(nc) as tc, ctx:
        const = ctx.enter_context(tc.tile_pool(name="const", bufs=1))
        big = ctx.enter_context(tc.tile_pool(name="big", bufs=1))
        work = ctx.enter_context(tc.tile_pool(name="work", bufs=2))
        scn = ctx.enter_context(tc.tile_pool(name="scn", bufs=1))
        psM = ctx.enter_context(tc.tile_pool(name="psM", bufs=2, space="PSUM"))
        psY = ctx.enter_context(tc.tile_pool(name="psY", bufs=2, space="PSUM"))

        def load2(name, rows, cols, dt=f32):
            t0 = const.tile([DT0, cols], dt, tag=name + "0", name=name + "0")
            t1 = const.tile([DT1, cols], dt, tag=name + "1", name=name + "1")
            nc.sync.dma_start(t0[:], din[name][0:DT0])
            nc.sync.dma_start(t1[:], din[name][DT0:rows])
            return t0, t1

        # input data first so the proj chain isn't stuck behind const loads
        xc0 = big.tile([DT0, L], bf16, tag="xc0")
        xc1 = big.tile([DT1, L], bf16, tag="xc1")
        nc.sync.dma_start(xc0[:], din["xc_t"][0:DT0])
        nc.sync.dma_start(xc1[:], din["xc_t"][DT0:CIN])
        projW0 = const.tile([DT0, COUT], bf16)
        projW1 = const.tile([DT1, COUT], bf16)
        nc.sync.dma_start(projW0[:], din["projW"][0:DT0])
        nc.sync.dma_start(projW1[:], din["projW"][DT0:CIN])
        projb = const.tile([COUT, 1], f32)
        nc.sync.dma_start(projb[:], din["projb"][:])
        W1t = const.tile([COUT, 2 * DIN], bf16)
        nc.sync.dma_start(W1t[:], din["W1"][:])
        b1x0 = const.tile([DT0, 1], f32); nc.sync.dma_start(b1x0[:], din["b1"][0:128])
        b1x1 = const.tile([DT1, 1], f32); nc.sync.dma_start(b1x1[:], din["b1"][128:192])
        b1z0 = const.tile([DT0, 1], f32); nc.sync.dma_start(b1z0[:], din["b1"][192:320])
        b1z1 = const.tile([DT1, 1], f32); nc.sync.dma_start(b1z1[:], din["b1"][320:384])
        cdiag0 = const.tile([DT0, 9, DT0], bf16)
        nc.sync.dma_start(cdiag0[:], din["cdiag0"][:])
        cdiag1 = const.tile([DT1, 9, DT1], bf16)
        nc.sync.dma_start(cdiag1[:], din["cdiag1"][:])
        convb0, convb1 = load2("convb", DIN, 1)
        cdiag1d = const.tile([DT1, 9, 128], bf16)
        nc.sync.dma_start(cdiag1d[:], din["cdiag1d"][:])
        cbd = const.tile([128, 1], f32)
        nc.sync.dma_start(cbd[:], din["cbd"][:])
        red1 = const.tile([128, DT1], bf16)
        nc.sync.dma_start(red1[:], din["red1"][:])
        eye = const.tile([128, 128], bf16)
        nc.sync.dma_start(eye[:], din["eye"][:])
        dsum0, dsum1 = load2("dsum", DIN, 1)
        mrow0, mrow1 = load2("mrow", DIN, 1)
        mcol0, mcol1 = load2("mcol", DIN, 1)
        kw = []
        for k in range(2):
            xp0 = const.tile([DT0, RNK + 2 * NST], bf16, name=f"xp{k}0")
            xp1 = const.tile([128, RNK + 2 * NST], bf16, name=f"xp{k}1")
            nc.sync.dma_start(xp0[:], din["xpw"][k, 0:DT0])
            nc.sync.dma_start(xp1[:], din["xpz"][k])
            dtw = const.tile([38, DIN], bf16, tag="dtwm", name=f"dtw{k}",
                             bufs=1) if k == 0 else kw[0]["dtwt"]
            nc.sync.dma_start(dtw[k * 32:k * 32 + RNK], din["dtw"][k])
            dtwd = const.tile([38, 128], bf16, tag="dtwdm", name=f"dtwd{k}",
                              bufs=1) if k == 0 else kw[0]["dtwdt"]
            nc.sync.dma_start(dtwd[k * 32:k * 32 + RNK], din["dtwd"][k])
            dtb0 = const.tile([DT0, 1], f32, name=f"dtb{k}0")
            dtb1 = const.tile([128, 1], f32, name=f"dtb{k}1")
            nc.sync.dma_start(dtb0[:], din["dtb"][k, 0:DT0])
            nc.sync.dma_start(dtb1[:], din["dtbd"][k])
            ac0 = const.tile([DT0, NST], f32, name=f"ac{k}0")
            ac1 = const.tile([128, 8], f32, name=f"ac{k}1")
            nc.sync.dma_start(ac0[:], din["acoef"][k, 0:DT0])
            nc.sync.dma_start(ac1[:], din["acp"][k])
            kw.append(dict(xp=(xp0, xp1), dtwt=dtw, dtwdt=dtwd,
                           dtw=dtw[k * 32:k * 32 + RNK],
                           dtwd=dtwd[k * 32:k * 32 + RNK],
                           dtb=(dtb0, dtb1),
                           ac=(ac0, ac1)))

        ones128 = const.tile([128, 1], f32); nc.vector.memset(ones128[:], 1.0)
        onesrow = const.tile([1, 128], bf16); nc.vector.memset(onesrow[:], 1.0)
        epsc = const.tile([1, 1], f32); nc.vector.memset(epsc[:], EPS)

        # ---- proj (f32r matmuls, x_t kept fp32 for residual) ----
        x_t = big.tile([COUT, L], f32, tag="x_t")
        for (s, w) in MM:
            ps = psM.tile([128, MMC], f32, tag="mm", name="psproj")
            nc.tensor.matmul(ps[:COUT, :w], projW0[:], xc0[:, s:s + w],
                             start=True, stop=False)
            nc.tensor.matmul(ps[:COUT, :w], projW1[:], xc1[:, s:s + w],
                             start=False, stop=True)
            nc.scalar.activation(x_t[:, s:s + w], ps[:COUT, :w], AF.Identity,
                                 bias=projb[:])
        nc.sync.dma_start(ox_d[:], x_t[:])

        # ---- LN1 (Copy + Sqrt share the act-table phase) -> xn bf16 ----
        xn_t = big.tile([COUT, L], bf16, tag="xn")
        for (s, w) in MM:
            ps1 = psM.tile([128, MMC], f32, tag="mm", name="pss1")
            nc.tensor.matmul(ps1[:1, :w], ones128[:COUT], x_t[:, s:s + w],
                             start=True, stop=True)
            mrw = work.tile([1, MMC], bf16, tag="mrw", bufs=1)
            nc.scalar.activation(mrw[:, :w], ps1[:1, :w], AF.Copy, scale=1.0 / COUT)
            sq = work.tile([128, MMC], f32, tag="sqc", bufs=1)
            nc.vector.tensor_tensor(out=sq[:COUT, :w], in0=x_t[:, s:s + w],
                                    in1=x_t[:, s:s + w], op=OP_.mult)
            ps2 = psM.tile([128, MMC], f32, tag="mm", name="pss2")
            nc.tensor.matmul(ps2[:1, :w], ones128[:COUT], sq[:COUT, :w],
                             start=True, stop=True)
            mq = work.tile([1, MMC], f32, tag="mq", bufs=1)
            nc.scalar.activation(mq[:, :w], ps2[:1, :w], AF.Copy, scale=1.0 / COUT)
            msq = work.tile([1, MMC], f32, tag="msq", bufs=1)
            nc.vector.tensor_tensor(out=msq[:, :w], in0=mrw[:, :w],
                                    in1=mrw[:, :w], op=OP_.mult)
            nc.vector.tensor_tensor(out=mq[:, :w], in0=mq[:, :w],
                                    in1=msq[:, :w], op=OP_.subtract)
            nc.scalar.activation(mq[:, :w], mq[:, :w], AF.Sqrt, bias=epsc[:])
            rsw = work.tile([1, MMC], bf16, tag="rsw", bufs=1)
            with nc.allow_low_precision(reason="bf16 1/std is well conditioned"):
                nc.vector.reciprocal(rsw[:, :w], mq[:, :w])
            pm = psM.tile([128, MMC], f32, tag="mm", name="psbm")
            nc.tensor.matmul(pm[:, :w], onesrow[:], mrw[:, :w],
                             start=True, stop=True)
            pr = psM.tile([128, MMC], f32, tag="mm", name="psbr")
            nc.tensor.matmul(pr[:, :w], onesrow[:], rsw[:, :w],
                             start=True, stop=True)
            xn_ = work.tile([128, MMC], bf16, tag="xn_", bufs=1)
            nc.vector.tensor_tensor(out=xn_[:COUT, :w], in0=x_t[:, s:s + w],
                                    in1=pm[:COUT, :w], op=OP_.subtract)
            nc.vector.tensor_tensor(out=xn_t[:, s:s + w], in0=xn_[:COUT, :w],
                                    in1=pr[:COUT, :w], op=OP_.mult)

        # ---- in_proj (xm tiles bf16; z silu'd -> DRAM bf16) ----
        xm0 = big.tile([DT0, L], bf16, tag="xm0")
        xm1 = big.tile([DT1, L], bf16, tag="xm1")
        for (s, w) in MM:
            for (coff, rows, bcol, dst, zoff) in (
                    (0, DT0, b1x0, xm0, None), (DT0, DT1, b1x1, xm1, None),
                    (DIN, DT0, b1z0, None, 0), (DIN + DT0, DT1, b1z1, None, DT0)):
                psi = psM.tile([128, MMC], f32, tag="mm", name="psip")
                nc.tensor.matmul(psi[:rows, :w], W1t[:, coff:coff + rows],
                                 xn_t[:, s:s + w], start=True, stop=True)
                if dst is not None:
                    nc.scalar.activation(dst[:, s:s + w], psi[:rows, :w],
                                         AF.Identity, bias=bcol[:])
                else:
                    zc = work.tile([128, MMC], bf16, tag="zc", bufs=1)
                    nc.scalar.activation(zc[:rows, :w], psi[:rows, :w], AF.Silu,
                                         bias=bcol[:])
                    nc.sync.dma_start(oz_d[zoff:zoff + rows, s:s + w], zc[:rows, :w])

        # ---- depthwise conv via PE diag matmuls + fused SiLU; the
        #      transposed copy for the plane transform happens per row-chunk
        xs0 = big.tile([DT0, L], bf16, tag="xs0")
        xs1 = big.tile([128, L], bf16, tag="xs1")
        xt0 = big.tile([DT0, L], bf16, tag="xm0", name="xt0")
        xt1 = big.tile([128, L], bf16, tag="xm1", name="xt1")
        for (src, cd, srows, orows, out, bcol, xtt, teng) in (
                (xm0, cdiag0, DT0, DT0, xs0, convb0, xt0, nc.vector),
                (xm1, cdiag1d, DT1, 128, xs1, cbd, xt1, nc.vector)):
            pad = big.tile([128, 50, 50], bf16, tag="xc0", name="pad")
            nc.vector.memset(pad[:srows], 0.0)
            r0 = 0
            for (s, w) in MM:
                r1 = (s + w) // 48
                nc.vector.tensor_copy(out=pad[:srows, 1 + r0:1 + r1, 1:49],
                                      in_=_pl3(src[:])[:, r0:r1, :])
                r0 = r1
            for (y0, ny) in CROWS:
                pc = psM.tile([128, MMC], f32, tag="mm", name="pscv")
                for j in range(9):
                    dy, dx = divmod(j, 3)
                    view = pad[:srows, y0 + dy:y0 + dy + ny, dx:dx + 48]
                    nc.tensor.matmul(pc[:orows, :ny * 48], cd[:, j], view,
                                     start=(j == 0), stop=(j == 8))
                nc.scalar.activation(out[:, y0 * 48:(y0 + ny) * 48],
                                     pc[:orows, :ny * 48], AF.Silu, bias=bcol[:])
                # xt[p, x, y] = xs[p, y, x] for this y-chunk
                xin = _pl3(out[:])[:, y0:y0 + ny, :]
                xout = bass.AP(tensor=xtt.tensor, offset=xtt[:].offset + y0,
                               ap=[xtt[:].ap[0], [1, ny], [48, 48]])
                teng.tensor_copy(out=xout, in_=xin)
        nc.vector.tensor_scalar_mul(xs0[:], xs0[:], mrow0[:])
        nc.vector.scalar_tensor_tensor(out=xs0[:], in0=xt0[:], scalar=mcol0[:],
                                       in1=xs0[:], op0=OP_.mult, op1=OP_.add)
        nc.vector.tensor_scalar_mul(xs1[:], xs1[:], mrow0[:])
        nc.vector.scalar_tensor_tensor(out=xs1[:], in0=xt1[:], scalar=mcol0[:],
                                       in1=xs1[:], op0=OP_.mult, op1=OP_.add)

        # ---- U96: x_dbl for both k; B/C staged to DRAM bf16 ----
        u96m = big.tile([38, L], bf16, tag="u96r")
        u96rs = (u96m[0:RNK], u96m[32:38])
        for k in range(2):
            W = kw[k]
            rb = k * 32          # rank section base: 0 (k0) / 32 (k1)
            bb = rb + 32
            cb = 64 if k == 0 else 0
            for (s, w) in MM:
                ps = psM.tile([128, MMC], f32, tag="mm", name="psU")
                for (coff, ubase, m) in ((0, rb, RNK), (RNK, bb, NST),
                                         (RNK + NST, cb, NST)):
                    nc.tensor.matmul(ps[ubase:ubase + m, :w],
                                     W["xp"][0][:, coff:coff + m],
                                     xs0[:, s:s + w], start=True, stop=False)
                    nc.tensor.matmul(ps[ubase:ubase + m, :w],
                                     W["xp"][1][:, coff:coff + m],
                                     xs1[:, s:s + w], start=False, stop=True)
                nc.vector.tensor_copy(out=u96rs[k][:, s:s + w],
                                       in_=ps[rb:rb + RNK, :w])
                bcsw = work.tile([112, MMC], bf16, tag="bcsw")
                nc.vector.tensor_copy(out=bcsw[bb:bb + NST, :w],
                                      in_=ps[bb:bb + NST, :w])
                nc.vector.tensor_copy(out=bcsw[cb:cb + NST, :w],
                                      in_=ps[cb:cb + NST, :w])
                nc.sync.dma_start(bcd[k, 0:NST, s:s + w], bcsw[bb:bb + NST, :w])
                nc.sync.dma_start(bcd[k, NST:32, s:s + w], bcsw[cb:cb + NST, :w])

        # ---- delta (Softplus) for both k, both dt ----
        dcs = []
        for k in range(2):
            W = kw[k]
            dc0 = big.tile([DT0, L], bf16, tag="xc0" if k == 0 else "xc1",
                           name=f"dc{k}0")
            dc1 = big.tile([128, L], bf16, tag=f"dc{k}1", name=f"dc{k}1")
            for (s, w) in MM:
                for dt, (rows, stat, dct) in enumerate(
                        ((DT0, W["dtw"][:, 0:DT0], dc0),
                         (128, W["dtwd"], dc1))):
                    ps = psM.tile([128, MMC], f32, tag="mm", name="psdt")
                    nc.tensor.matmul(ps[:rows, :w], stat,
                                     u96rs[k][:, s:s + w],
                                     start=True, stop=True)
                    nc.scalar.activation(dct[:, s:s + w], ps[:rows, :w],
                                         AF.Exp, bias=W["dtb"][dt][:])
            # in-place Ln (softplus) + dxc = delta*x, per chunk
            dx0 = big.tile([DT0, L], bf16, tag="x_t" if k == 0 else "xn",
                           name=f"dx{k}0")
            dx1 = big.tile([128, L], bf16, tag="xm1" if k == 0 else "u96r",
                           name=f"dx{k}1")
            for (s, w) in MM:
                for dct, dxt, xst in ((dc0, dx0, xs0), (dc1, dx1, xs1)):
                    nc.scalar.activation(dct[:, s:s + w], dct[:, s:s + w],
                                         AF.Ln, bias=1.0)
                    nc.vector.tensor_tensor(out=dxt[:, s:s + w],
                                            in0=dct[:, s:s + w],
                                            in1=xst[:, s:s + w], op=OP_.mult)
            dcs.append((dc0, dc1, dx0, dx1))

        # ---- P init with the direction-independent D term ----
        P0 = big.tile([DT0, L], bf16, tag="xm0", name="P0")
        P1 = big.tile([DT1, L], bf16, tag="xt1m", name="P1")
        nc.vector.tensor_scalar_mul(P0[:], xs0[:], dsum0[:])
        nc.gpsimd.tensor_scalar_mul(P1[:], xs1[0:DT1], dsum1[:])

        # ---- selective scan ----
        hp = [[scn.tile([128, NST if dt == 0 else 8, 1], bf16,
                        tag=f"hp{k}{dt}", name=f"hp{k}{dt}")
               for dt in range(2)] for k in range(2)]
        for k in range(2):
            for dt in range(2):
                nc.vector.memset(hp[k][dt][:], 0.0)

        pairs = [(c, k) for c in range(len(SC)) for k in range(2)]
        bcr_tiles = {}

        def issue_bcast(i):
            c, k = pairs[i]
            s, w = SC[c]
            s0 = s if k == 0 else L - s - w
            t = scn.tile([128, 40, LC], bf16, tag="bcr", bufs=3,
                         name=f"bcr{i}")
            src = bass.AP(tensor=bcd, offset=k * 32 * L + s0,
                          ap=[[0, 128], [L, 32], [1, w]])
            nc.sync.dma_start(t[:, 0:32], src)
            # packed dt1 C': rows 0:64 get states 16:24, rows 64:128 get 24:32
            for (p0, srow) in ((0, 16), (64, 24)):
                sp = bass.AP(tensor=bcd, offset=k * 32 * L + srow * L + s0,
                             ap=[[0, 64], [L, 8], [1, w]])
                nc.sync.dma_start(t[p0:p0 + 64, 32:40, :], sp)
            bcr_tiles[i] = t

        pendingP = []

        def tail(dt, dA, Ht, crep, hpt, pdst):
            # after the scan: save carry, G = H*C, PE n-reduction; the P
            # accumulate is deferred further so DVE never waits on the PE
            if len(pendingP) >= 2:
                pendingP.pop(0)()
            nc.gpsimd.tensor_copy(out=hpt[:], in_=Ht[:, :, LC:LC + 1])
            psy = psY.tile([128, LC], f32, tag="psy", name="psy", bufs=3)
            if dt == 0:
                nc.vector.tensor_tensor(out=dA[:, :, 1:], in0=Ht[:, :, 1:],
                                        in1=crep, op=OP_.mult)
                for n in range(NST):
                    nc.tensor.matmul(psy[:, :], eye[:, :],
                                     dA[:, n, 1:],
                                     start=(n == 0), stop=(n == NST - 1))
                rows = DT0
            else:
                nc.vector.tensor_tensor(out=dA[:, :, 1:],
                                        in0=Ht[:, :, 1:],
                                        in1=crep, op=OP_.mult)
                for n in range(8):
                    nc.tensor.matmul(psy[:DT1, :], red1[:, :],
                                     dA[:, n, 1:],
                                     start=(n == 0), stop=(n == 7))
                rows = DT1

            def paccum(psy=psy, rows=rows, pdst=pdst):
                nc.vector.tensor_tensor(out=pdst, in0=psy[:rows, :], in1=pdst,
                                        op=OP_.add)
            pendingP.append(paccum)

        pending = []
        issue_bcast(0)
        for i, (c, k) in enumerate(pairs):
            s, w = SC[c]
            W = kw[k]
            bcr = bcr_tiles.pop(i)
            if k == 0:
                brep = bcr[:, 0:NST, :]
                crep = bcr[:, NST:32, :]
            else:
                brep = bcr[:, 0:NST, ::-1]
                crep = bcr[:, NST:32, ::-1]
            for dt in range(2):
                ns = NST if dt == 0 else 8
                if len(pending) >= 3:
                    # flush the 3-iterations-old deferred tail before its
                    # dA/dBu/Ht buffer slots are reused below
                    pending.pop(0)()
                if dt == 0 and i + 1 < len(pairs):
                    # safe point: all readers of bcr slot (i+1)%3's previous
                    # occupant (pair i-2) have been emitted by now
                    issue_bcast(i + 1)
                dct = dcs[k][dt]
                dcsl = _sl(dct[:], k, s, w)
                dxsl = _sl(dcs[k][2 + dt][:], k, s, w)
                dA = scn.tile([128, ns, LC + 1], bf16, tag="dA",
                              name=f"dA{dt}", bufs=3)
                dBu = scn.tile([128, ns, LC + 1], bf16, tag="dBu",
                               name=f"dBu{dt}", bufs=2)
                Ht = scn.tile([128, ns, LC + 1], bf16, tag="Ht",
                              name=f"Ht{dt}", bufs=3)
                nc.gpsimd.memset(dA[:, :, 0:1], 0.0)
                nc.gpsimd.tensor_copy(out=dBu[:, :, 0:1], in_=hp[k][dt][:])
                for n in range(ns):
                    nc.scalar.activation(dA[:, n, 1:], dcsl, AF.Exp,
                                         scale=W["ac"][dt][:, n:n + 1])
                beng = nc.vector if dt == 0 else nc.gpsimd
                if dt == 0:
                    beng.tensor_tensor(out=dBu[:, :, 1:],
                                       in0=_rep(dxsl, NST),
                                       in1=brep, op=OP_.mult)
                else:
                    # packed: nh half selects B rows 0:8 / 8:16
                    beng.tensor_tensor(out=dBu[0:DT1, :, 1:],
                                       in0=_rep(dxsl[0:DT1], 8),
                                       in1=brep[0:DT1, 0:8], op=OP_.mult)
                    beng.tensor_tensor(out=dBu[DT1:128, :, 1:],
                                       in0=_rep(dxsl[DT1:128], 8),
                                       in1=brep[DT1:128, 8:NST], op=OP_.mult)
                nc.vector.tensor_tensor_scan(
                    out=Ht[:].rearrange("p a b -> p (a b)"),
                    data0=dA[:].rearrange("p a b -> p (a b)"),
                    data1=dBu[:].rearrange("p a b -> p (a b)"),
                    initial=0.0, op0=OP_.mult, op1=OP_.add)
                Pt = P0 if dt == 0 else P1
                cr = crep if dt == 0 else (
                    bcr[:, 32:40, :] if k == 0 else bcr[:, 32:40, ::-1])
                args = (dt, dA, Ht, cr, hp[k][dt], _sl(Pt[:], k, s, w))
                pending.append(lambda a=args: tail(*a))
        for fn in pending:
            fn()
        for fn in pendingP:
            fn()

        # ---- Q = mrow*P + mcol*transpose(P) ----
        Q0 = big.tile([DT0, L], bf16, tag="xs0", name="Q0")
        Q1 = big.tile([DT1, L], bf16, tag="xs1", name="Q1")
        nc.vector.tensor_scalar_mul(Q0[:], _twh(P0[:]), mcol0[:])
        nc.vector.scalar_tensor_tensor(out=Q0[:], in0=P0[:], scalar=mrow0[:],
                                       in1=Q0[:], op0=OP_.mult, op1=OP_.add)
        nc.gpsimd.tensor_scalar_mul(Q1[:], _twh(P1[:]), mcol1[:])
        qtm = big.tile([DT1, L], bf16, tag="xm1", name="qtm")
        nc.gpsimd.tensor_scalar_mul(qtm[:], P1[:], mrow1[:])
        nc.gpsimd.tensor_tensor(out=Q1[:], in0=qtm[:], in1=Q1[:], op=OP_.add)
        nc.sync.dma_start(oq_d[0:DT0], Q0[:])
        nc.sync.dma_start(oq_d[DT0:DIN], Q1[:])
    nc.compile()
    return nc


# ---------------------------------------------------------------- pass 2
def build_nc2():
    nc = bacc.Bacc("TRN2", target_bir_lowering=False, debug=False, num_devices=8)
    din = {}

    def I(name, shape, dt=f32):
        din[name] = nc.dram_tensor(name, shape, dt, kind="ExternalInput")

    I("ym", [DIN, L], bf16); I("xin", [COUT, L]); I("zin", [DIN, L], bf16)
    I("OPm", [DIN, COUT], bf16); I("OPB", [DIN, COUT], bf16)
    I("PW1", [COUT, HID], bf16); I("g1", [HID, 1]); I("bb1", [HID, 1])
    I("cbdiag0", [DT0, 9, DT0], bf16); I("cbdiag1", [DT1, 9, DT1], bf16)
    I("g2", [HID, 1]); I("bb2", [HID, 1])
    I("PW2", [HID, COUT], bf16); I("g3", [COUT, 1]); I("bb3", [COUT, 1])
    I("fw", [COUT, 1]); I("fb", [COUT, 1])
    out_d = nc.dram_tensor("o", [COUT, L], f32, kind="ExternalOutput")

    ctx = contextlib.ExitStack()
    with tile.TileContext(nc) as tc, ctx:
        const = ctx.enter_context(tc.tile_pool(name="const", bufs=1))
        big = ctx.enter_context(tc.tile_pool(name="big", bufs=1))
        work = ctx.enter_context(tc.tile_pool(name="work", bufs=2))
        psM = ctx.enter_context(tc.tile_pool(name="psM", bufs=2, space="PSUM"))

        def load2(name, rows, cols, dt=f32):
            t0 = const.tile([DT0, cols], dt, tag=name + "0", name=name + "0")
            t1 = const.tile([DT1, cols], dt, tag=name + "1", name=name + "1")
            nc.sync.dma_start(t0[:], din[name][0:DT0])
            nc.sync.dma_start(t1[:], din[name][DT0:rows])
            return t0, t1

        def load1(name, rows):
            t = const.tile([rows, 1], f32, tag=name, name=name)
            nc.sync.dma_start(t[:], din[name][:])
            return t

        # input data first so the out-norm chain isn't stuck behind consts
        ym0 = big.tile([DT0, L], bf16, tag="ym0")
        ym1 = big.tile([DT1, L], bf16, tag="ym1")
        nc.sync.dma_start(ym0[:], din["ym"][0:DT0])
        nc.sync.dma_start(ym1[:], din["ym"][DT0:DIN])
        xres = big.tile([COUT, L], f32, tag="xres")
        nc.sync.dma_start(xres[:], din["xin"][:])
        zc0 = big.tile([DT0, L], bf16, tag="zc0")
        zc1 = big.tile([DT1, L], bf16, tag="zc1")
        nc.sync.dma_start(zc0[:], din["zin"][0:DT0])
        nc.sync.dma_start(zc1[:], din["zin"][DT0:DIN])
        OP0, OP1 = load2("OPm", DIN, COUT, bf16)
        OPB0, OPB1 = load2("OPB", DIN, COUT, bf16)
        PW1t = const.tile([COUT, HID], bf16)
        nc.sync.dma_start(PW1t[:], din["PW1"][:])
        g1c0, g1c1 = load2("g1", HID, 1)
        bb1c0, bb1c1 = load2("bb1", HID, 1)
        cbd0 = const.tile([DT0, 9, DT0], bf16)
        nc.sync.dma_start(cbd0[:], din["cbdiag0"][:])
        cbd1 = const.tile([DT1, 9, DT1], bf16)
        nc.sync.dma_start(cbd1[:], din["cbdiag1"][:])
        g2c0, g2c1 = load2("g2", HID, 1)
        bb2c0, bb2c1 = load2("bb2", HID, 1)
        PW20, PW21 = load2("PW2", HID, COUT, bf16)
        g3c = load1("g3", COUT); bb3c = load1("bb3", COUT)
        fwc = load1("fw", COUT); fbc = load1("fb", COUT)
        onesb = const.tile([128, 1], bf16); nc.vector.memset(onesb[:], 1.0)
        onesrow = const.tile([1, 128], bf16); nc.vector.memset(onesrow[:], 1.0)
        epsc = const.tile([1, 1], f32); nc.vector.memset(epsc[:], EPS)

        # out-norm stats over 192 partitions (per-chunk)
        mean_r = big.tile([1, L], bf16, tag="mean")
        rs_r = big.tile([1, L], bf16, tag="rs")
        for (s, w) in MM:
            ps = psM.tile([128, MMC], f32, tag="mm", name="pso1")
            nc.tensor.matmul(ps[:1, :w], onesb[:], ym0[:, s:s + w],
                             start=True, stop=False)
            nc.tensor.matmul(ps[:1, :w], onesb[:DT1], ym1[:, s:s + w],
                             start=False, stop=True)
            nc.scalar.activation(mean_r[:, s:s + w], ps[:1, :w], AF.Copy,
                                 scale=1.0 / DIN)
            ps2 = psM.tile([128, MMC], f32, tag="mm", name="pso2")
            for i, (t, rows) in enumerate(((ym0, DT0), (ym1, DT1))):
                sq = work.tile([128, MMC], bf16, tag="sqc", bufs=1)
                nc.vector.tensor_tensor(out=sq[:rows, :w], in0=t[:, s:s + w],
                                        in1=t[:, s:s + w], op=OP_.mult)
                nc.tensor.matmul(ps2[:1, :w], onesb[:rows], sq[:rows, :w],
                                 start=(i == 0), stop=(i == 1))
            mq = work.tile([1, MMC], f32, tag="mq", bufs=1)
            nc.scalar.activation(mq[:, :w], ps2[:1, :w], AF.Copy,
                                 scale=1.0 / DIN)
            msqc = work.tile([1, MMC], f32, tag="msqc", bufs=1)
            nc.vector.tensor_tensor(out=msqc[:, :w], in0=mean_r[:, s:s + w],
                                    in1=mean_r[:, s:s + w], op=OP_.mult)
            nc.vector.tensor_tensor(out=mq[:, :w], in0=mq[:, :w],
                                    in1=msqc[:, :w], op=OP_.subtract)
            nc.scalar.activation(mq[:, :w], mq[:, :w], AF.Sqrt, bias=epsc[:])
            with nc.allow_low_precision(reason="bf16 1/std is well conditioned"):
                nc.vector.reciprocal(rs_r[:, s:s + w], mq[:, :w])

        x2f = big.tile([COUT, L], f32, tag="x2f")
        x2b = big.tile([COUT, L], bf16, tag="x2b")
        for (s, w) in MM:
            pm = psM.tile([128, MMC], f32, tag="mm", name="psm")
            nc.tensor.matmul(pm[:, :w], onesrow[:], mean_r[:, s:s + w],
                             start=True, stop=True)
            pr = psM.tile([128, MMC], f32, tag="mm", name="psr")
            nc.tensor.matmul(pr[:, :w], onesrow[:], rs_r[:, s:s + w],
                             start=True, stop=True)
            po = psM.tile([128, MMC], f32, tag="mm", name="pso")
            for i, (t, z, rows) in enumerate(((ym0, zc0, DT0), (ym1, zc1, DT1))):
                yn = work.tile([128, MMC], bf16, tag=f"yn{i}", name=f"yn{i}")
                nc.vector.tensor_tensor(out=yn[:rows, :w], in0=t[:, s:s + w],
                                        in1=pm[:rows, :w], op=OP_.subtract)
                nc.vector.tensor_tensor(out=yn[:rows, :w], in0=yn[:rows, :w],
                                        in1=pr[:rows, :w], op=OP_.mult)
                nc.vector.tensor_tensor(out=yn[:rows, :w], in0=yn[:rows, :w],
                                        in1=z[:, s:s + w], op=OP_.mult)
                OPt = OP0 if i == 0 else OP1
                OPBt = OPB0 if i == 0 else OPB1
                nc.tensor.matmul(po[:COUT, :w], OPt[:], yn[:rows, :w],
                                 start=(i == 0), stop=False)
                nc.tensor.matmul(po[:COUT, :w], OPBt[:], z[:, s:s + w],
                                 start=False, stop=(i == 1))
            nc.vector.tensor_tensor(out=x2f[:, s:s + w], in0=po[:COUT, :w],
                                    in1=xres[:, s:s + w], op=OP_.add)
            nc.scalar.activation(x2b[:, s:s + w], x2f[:, s:s + w], AF.Copy)

        # ConvBlock: PW1 + gelu
        t0 = big.tile([DT0, L], bf16, tag="ym0", name="t0")
        t1 = big.tile([DT1, L], bf16, tag="ym1", name="t1")
        for (s, w) in MM:
            for (dst, coff, rows, gc_, bc_) in ((t0, 0, DT0, g1c0, bb1c0),
                                                (t1, DT0, DT1, g1c1, bb1c1)):
                ps = psM.tile([128, MMC], f32, tag="mm", name="psp1")
                nc.tensor.matmul(ps[:rows, :w], PW1t[:, coff:coff + rows],
                                 x2b[:, s:s + w], start=True, stop=True)
                nc.scalar.activation(dst[:, s:s + w], ps[:rows, :w], AF.Gelu,
                                     bias=bc_[:], scale=gc_[:])
        # dw conv via PE; fused bn2+gelu on psum
        v0 = big.tile([DT0, L], bf16, tag="zc0", name="v0")
        v1 = big.tile([DT1, L], bf16, tag="zc1", name="v1")
        for (src, cd, rows, out, gc_, bc_) in (
                (t0, cbd0, DT0, v0, g2c0, bb2c0),
                (t1, cbd1, DT1, v1, g2c1, bb2c1)):
            pad = work.tile([128, 50, 50], bf16, tag="pad", bufs=1)
            nc.vector.memset(pad[:rows], 0.0)
            r0 = 0
            for (s, w) in MM:
                r1 = (s + w) // 48
                nc.vector.tensor_copy(out=pad[:rows, 1 + r0:1 + r1, 1:49],
                                      in_=_pl3(src[:])[:, r0:r1, :])
                r0 = r1
            for (y0, ny) in CROWS:
                pc = psM.tile([128, MMC], f32, tag="mm", name="pscv")
                for j in range(9):
                    dy, dx = divmod(j, 3)
                    view = pad[:rows, y0 + dy:y0 + dy + ny, dx:dx + 48]
                    nc.tensor.matmul(pc[:rows, :ny * 48], cd[:, j], view,
                                     start=(j == 0), stop=(j == 8))
                nc.scalar.activation(out[:, y0 * 48:(y0 + ny) * 48],
                                     pc[:rows, :ny * 48], AF.Gelu,
                                     bias=bc_[:], scale=gc_[:])
        # PW2 + bn3 + residual
        x3f = big.tile([COUT, L], f32, tag="x3f")
        x3b = big.tile([COUT, L], bf16, tag="xres", name="x3b")
        for (s, w) in MM:
            ps = psM.tile([128, MMC], f32, tag="mm", name="psp2")
            nc.tensor.matmul(ps[:COUT, :w], PW20[:], v0[:, s:s + w],
                             start=True, stop=False)
            nc.tensor.matmul(ps[:COUT, :w], PW21[:], v1[:, s:s + w],
                             start=False, stop=True)
            cbt = work.tile([128, MMC], bf16, tag="cbt", bufs=1)
            nc.scalar.activation(cbt[:COUT, :w], ps[:COUT, :w], AF.Identity,
                                 bias=bb3c[:], scale=g3c[:])
            nc.vector.tensor_tensor(out=x3f[:, s:s + w], in0=cbt[:COUT, :w],
                                    in1=x2f[:, s:s + w], op=OP_.add)
            nc.scalar.activation(x3b[:, s:s + w], x3f[:, s:s + w], AF.Copy)

        # final LN
        mean2 = big.tile([1, L], bf16, tag="mean2")
        rs2 = big.tile([1, L], bf16, tag="rs2")
        for (s, w) in MM:
            ps = psM.tile([128, MMC], f32, tag="mm", name="psf1")
            nc.tensor.matmul(ps[:1, :w], onesb[:COUT], x3b[:, s:s + w],
                             start=True, stop=True)
            nc.scalar.activation(mean2[:, s:s + w], ps[:1, :w], AF.Copy,
                                 scale=1.0 / COUT)
            sq = work.tile([128, MMC], bf16, tag="sqc", bufs=1)
            nc.vector.tensor_tensor(out=sq[:COUT, :w], in0=x3b[:, s:s + w],
                                    in1=x3b[:, s:s + w], op=OP_.mult)
            ps2 = psM.tile([128, MMC], f32, tag="mm", name="psf2")
            nc.tensor.matmul(ps2[:1, :w], onesb[:COUT], sq[:COUT, :w],
                             start=True, stop=True)
            mq2 = work.tile([1, MMC], f32, tag="mq2", bufs=1)
            nc.scalar.activation(mq2[:, :w], ps2[:1, :w], AF.Copy,
                                 scale=1.0 / COUT)
            msqc2 = work.tile([1, MMC], f32, tag="msqc2", bufs=1)
            nc.vector.tensor_tensor(out=msqc2[:, :w], in0=mean2[:, s:s + w],
                                    in1=mean2[:, s:s + w], op=OP_.mult)
            nc.vector.tensor_tensor(out=mq2[:, :w], in0=mq2[:, :w],
                                    in1=msqc2[:, :w], op=OP_.subtract)
            nc.scalar.activation(mq2[:, :w], mq2[:, :w], AF.Sqrt, bias=epsc[:])
            with nc.allow_low_precision(reason="bf16 1/std is well conditioned"):
                nc.vector.reciprocal(rs2[:, s:s + w], mq2[:, :w])
        for (s, w) in MM:
            pm = psM.tile([128, MMC], f32, tag="mm", name="psfm")
            nc.tensor.matmul(pm[:, :w], onesrow[:], mean2[:, s:s + w],
                             start=True, stop=True)
            pr = psM.tile([128, MMC], f32, tag="mm", name="psfr")
            nc.tensor.matmul(pr[:, :w], onesrow[:], rs2[:, s:s + w],
                             start=True, stop=True)
            oc = work.tile([128, MMC], f32, tag="oc", bufs=1)
            nc.vector.tensor_tensor(out=oc[:COUT, :w], in0=x3f[:, s:s + w],
                                    in1=pm[:COUT, :w], op=OP_.subtract)
            nc.vector.tensor_tensor(out=oc[:COUT, :w], in0=oc[:COUT, :w],
                                    in1=pr[:COUT, :w], op=OP_.mult)
            nc.vector.tensor_scalar(out=oc[:COUT, :w], in0=oc[:COUT, :w],
                                    scalar1=fwc[:], scalar2=fbc[:],
                                    op0=OP_.mult, op1=OP_.add)
            nc.sync.dma_start(out_d[:, s:s + w], oc[:COUT, :w])
    nc.compile()
    return nc


_NC1, _NC2 = None, None


def _get_ncs():
    global _NC1, _NC2
    if _NC1 is None:
        _NC1 = build_nc1()
        _NC2 = build_nc2()
    return _NC1, _NC2


def _bf(a):
    import jax.numpy as jnp
    return np.asarray(jnp.asarray(np.asarray(a, np.float32), jnp.bfloat16))


def _diag9(wmat, rows):
    out = np.zeros((rows, 9, rows), np.float32)
    idx = np.arange(rows)
    for j in range(9):
        out[idx, j, idx] = wmat[:, j]
    return out


def prep_pass1(ip):
    W1 = (np.diag(ip["ln1_w"]) @ ip["in_proj_W"]).astype(np.float32)
    b1 = (ip["ln1_b"] @ ip["in_proj_W"] + ip["in_proj_b"]).astype(np.float32)
    A = (-np.exp(ip["A_logs"].astype(np.float64))).astype(np.float32).reshape(KDIR, DIN, NST)
    Ds = ip["Ds"].reshape(KDIR, DIN)
    col = lambda v: np.ascontiguousarray(v.reshape(-1, 1), dtype=np.float32)
    convW = ip["conv_W"].reshape(DIN, 9)
    base = dict(projW=_bf(ip["proj_W"]), projb=col(ip["proj_b"]), W1=_bf(W1),
                b1=col(b1),
                cdiag0=_bf(_diag9(convW[0:DT0], DT0)),
                cdiag1=_bf(_diag9(convW[DT0:DIN], DT1)),
                convb=col(ip["conv_b"]),
                eye=_bf(np.eye(128, dtype=np.float32)))
    # packed dt1 (channels 128:192 as p = d + 64*nh, 8 states per slot)
    cd1 = np.zeros((DT1, 9, 128), np.float32)
    di = np.arange(DT1)
    for j in range(9):
        cd1[di, j, di] = convW[DT0 + di, j]
        cd1[di, j, DT1 + di] = convW[DT0 + di, j]
    base["cdiag1d"] = _bf(cd1)
    base["cbd"] = col(np.tile(ip["conv_b"][DT0:], 2))
    base["red1"] = _bf(np.tile(np.eye(DT1, dtype=np.float32), (2, 1)))
    maps = []
    for c in range(8):
        b, plane = c // 2, c % 2
        ks = [plane, plane + 2]
        m = dict(base)
        m["xc_t"] = _bf(np.ascontiguousarray(ip["x_cat"][b].reshape(L, CIN).T))
        m["xpw"] = _bf(np.stack([ip["x_proj_W"][k].T for k in ks]))
        xpz = np.zeros((2, 128, RNK + 2 * NST), np.float32)
        for kk, k in enumerate(ks):
            xpz[kk, 0:DT1] = ip["x_proj_W"][k].T[DT0:DIN]
        m["xpz"] = _bf(xpz)
        m["dtw"] = _bf(np.stack([ip["dt_W"][k].T for k in ks]))
        m["dtwd"] = _bf(np.stack(
            [np.tile(ip["dt_W"][k].T[:, DT0:], (1, 2)) for k in ks]))
        m["dtb"] = np.ascontiguousarray(np.stack([col(ip["dt_b"][k]) for k in ks]))
        m["dtbd"] = np.ascontiguousarray(np.stack(
            [col(np.tile(ip["dt_b"][k][DT0:], 2)) for k in ks]))
        m["acoef"] = np.ascontiguousarray(np.stack([A[k] for k in ks]))
        acp = np.zeros((2, 128, 8), np.float32)
        for kk, k in enumerate(ks):
            for nh in range(2):
                acp[kk, nh * DT1:(nh + 1) * DT1, :] = A[k][DT0:DIN,
                                                           nh * 8:(nh + 1) * 8]
        m["acp"] = np.ascontiguousarray(acp)
        m["dsum"] = col(Ds[ks[0]] + Ds[ks[1]])
        m["mrow"] = np.full((DIN, 1), 1.0 - plane, np.float32)
        m["mcol"] = np.full((DIN, 1), float(plane), np.float32)
        maps.append(m)
    return maps


def prep_pass2(ip, res1):
    OPm = (np.diag(ip["out_norm_w"]) @ ip["out_proj_W"]).astype(np.float32)
    OPB = (np.diag(ip["out_norm_b"]) @ ip["out_proj_W"]).astype(np.float32)
    col = lambda v: np.ascontiguousarray(v.reshape(-1, 1), dtype=np.float32)
    cbw = ip["cb_dw_W"].reshape(HID, 9)
    base = dict(OPm=_bf(OPm), OPB=_bf(OPB),
                PW1=_bf(ip["cb_pw1_W"][:, :, 0, 0].T),
                g1=col(ip["cb_bn1_g"]), bb1=col(ip["cb_bn1_b"]),
                cbdiag0=_bf(_diag9(cbw[0:DT0], DT0)),
                cbdiag1=_bf(_diag9(cbw[DT0:HID], DT1)),
                g2=col(ip["cb_bn2_g"]), bb2=col(ip["cb_bn2_b"]),
                PW2=_bf(ip["cb_pw2_W"][:, :, 0, 0].T),
                g3=col(ip["cb_bn3_g"]), bb3=col(ip["cb_bn3_b"]),
                fw=col(ip["norm_w"]), fb=col(ip["norm_b"]))
    maps = []
    for c in range(8):
        b = c // 2
        m = dict(base)
        ymf = (np.asarray(res1[2 * b]["oq"], np.float32)
               + np.asarray(res1[2 * b + 1]["oq"], np.float32))
        m["ym"] = _bf(ymf)
        m["xin"] = np.asarray(res1[2 * b]["ox"], np.float32)
        m["zin"] = np.ascontiguousarray(res1[2 * b]["oz"])
        maps.append(m)
    return maps


def kernel(**inputs):
    ip = {k: np.asarray(v, np.float32) for k, v in inputs.items()}
    nc1, nc2 = _get_ncs()
    res1 = run_bass_kernel_spmd(nc1, prep_pass1(ip), list(range(8))).results
    res2 = run_bass_kernel_spmd(nc2, prep_pass2(ip, res1), list(range(8))).results
    outs = [np.asarray(res2[2 * b]["o"], np.float32).T.reshape(H_, W_, COUT)
            for b in range(B_)]
    return np.stack(outs).astype(np.float32)


# revision 49
# speedup vs baseline: 2.1824x; 1.0208x over previous
"""Trainium2 Bass kernel for nn_DecoderFusionBlock (VSS/Mamba decoder fusion block).

Two-pass SPMD over 8 cores:
  pass 1: core c -> batch b=c//2, plane=c%2 (row-/col-major spatial order).
          proj/LN/in_proj (f32r / bf16 matmuls), depthwise conv via PE diag
          matmuls, then the selective scan for the plane's two directions.
          bf16 data path with fp32 scan state; B/C broadcast to all channel
          partitions via a DRAM-staged broadcast DMA so the big elementwise
          multiplies run in the DVE 2x (2-byte) mode; the n-state reduction
          runs on the PE as identity-weight matmul accumulation in PSUM.
  host:   ym[b] = Q[2b] + Q[2b+1]  (the only cross-core reduction)
  pass 2: core c -> batch b=c//2: out-norm, gate, out_proj+residual,
          ConvBlock (conv again via PE), final LayerNorm.
"""

import contextlib
import numpy as np

import concourse.bass as bass
import concourse.tile as tile
from concourse import bacc, mybir
from concourse.bass_utils import run_bass_kernel_spmd

f32 = mybir.dt.float32
f32r = mybir.dt.float32r
bf16 = mybir.dt.bfloat16
AF = mybir.ActivationFunctionType
OP_ = mybir.AluOpType

B_, H_, W_ = 4, 48, 48
L = H_ * W_
CIN, COUT = 192, 96
DIN, NST, RNK, KDIR = 192, 16, 6, 4
HID = 192
EPS = 1e-5
DT0, DT1 = 128, 64
MMC = 512
MM = [(s, min(MMC, L - s)) for s in range(0, L, MMC)]
LC = 256
SC = [(i * LC, LC) for i in range(L // LC)]
CROWS = [(0, 10), (10, 10), (20, 10), (30, 10), (40, 8)]


def _rev(ap, s, w):
    hi = L - 1 - s
    lo = hi - w
    return ap[:, hi::-1] if lo < 0 else ap[:, hi:lo:-1]


def _sl(ap, k, s, w):
    return ap[:, s:s + w] if k == 0 else _rev(ap, s, w)


def _rep(a, n):
    return bass.AP(tensor=a.tensor, offset=a.offset, ap=[a.ap[0], [0, n], a.ap[1]])


def _twh(a):
    st = a.ap[1][0]
    return bass.AP(tensor=a.tensor, offset=a.offset,
                   ap=[a.ap[0], [st, 48], [48 * st, 48]])


def _pl3(a):
    st = a.ap[1][0]
    return bass.AP(tensor=a.tensor, offset=a.offset,
                   ap=[a.ap[0], [48 * st, 48], [st, 48]])


# ---------------------------------------------------------------- pass 1
def build_nc1():
    nc = bacc.Bacc("TRN2", target_bir_lowering=False, debug=False, num_devices=8)
    din = {}

    def I(name, shape, dt=f32):
        din[name] = nc.dram_tensor(name, shape, dt, kind="ExternalInput")

    I("xc_t", [CIN, L], bf16)
    I("projW", [CIN, COUT], bf16); I("projb", [COUT, 1])
    I("W1", [COUT, 2 * DIN], bf16); I("b1", [2 * DIN, 1])
    I("cdiag0", [DT0, 9, DT0], bf16); I("cdiag1", [DT1, 9, DT1], bf16)
    I("convb", [DIN, 1])
    I("eye", [128, 128], bf16)
    I("xpw", [2, DIN, RNK + 2 * NST], bf16)
    I("xpz", [2, 128, RNK + 2 * NST], bf16)
    I("dtw", [2, RNK, DIN], bf16); I("dtwd", [2, RNK, 128], bf16)
    I("dtb", [2, DIN, 1]); I("dtbd", [2, 128, 1])
    I("acoef", [2, DIN, NST]); I("acp", [2, 128, 8]); I("dsum", [DIN, 1])
    I("cdiag1d", [DT1, 9, 128], bf16); I("cbd", [128, 1])
    I("red1", [128, DT1], bf16)
    I("mrow", [DIN, 1]); I("mcol", [DIN, 1])
    oq_d = nc.dram_tensor("oq", [DIN, L], bf16, kind="ExternalOutput")
    ox_d = nc.dram_tensor("ox", [COUT, L], f32, kind="ExternalOutput")
    oz_d = nc.dram_tensor("oz", [DIN, L], bf16, kind="ExternalOutput")
    bcd = nc.dram_tensor("BCd", [2, 32, L], bf16, kind="Internal")

    ctx = contextlib.ExitStack()
    with tile.TileContext(nc) as tc, ctx:
        const = ctx.enter_context(tc.tile_pool(name="const", bufs=1))
        big = ctx.enter_context(tc.tile_pool(name="big", bufs=1))
        work = ctx.enter_context(tc.tile_pool(name="work", bufs=2))
        scn = ctx.enter_context(tc.tile_pool(name="scn", bufs=1))
        psM = ctx.enter_context(tc.tile_pool(name="psM", bufs=2, space="PSUM"))
        psY = ctx.enter_context(tc.tile_pool(name="psY", bufs=2, space="PSUM"))

        def load2(name, rows, cols, dt=f32):
            t0 = const.tile([DT0, cols], dt, tag=name + "0", name=name + "0")
            t1 = const.tile([DT1, cols], dt, tag=name + "1", name=name + "1")
            nc.sync.dma_start(t0[:], din[name][0:DT0])
            nc.sync.dma_start(t1[:], din[name][DT0:rows])
            return t0, t1

        # input data first so the proj chain isn't stuck behind const loads
        xc0 = big.tile([DT0, L], bf16, tag="xc0")
        xc1 = big.tile([DT1, L], bf16, tag="xc1")
        nc.sync.dma_start(xc0[:], din["xc_t"][0:DT0])
        nc.sync.dma_start(xc1[:], din["xc_t"][DT0:CIN])
        projW0 = const.tile([DT0, COUT], bf16)
        projW1 = const.tile([DT1, COUT], bf16)
        nc.sync.dma_start(projW0[:], din["projW"][0:DT0])
        nc.sync.dma_start(projW1[:], din["projW"][DT0:CIN])
        projb = const.tile([COUT, 1], f32)
        nc.sync.dma_start(projb[:], din["projb"][:])
        W1t = const.tile([COUT, 2 * DIN], bf16)
        nc.sync.dma_start(W1t[:], din["W1"][:])
        b1x0 = const.tile([DT0, 1], f32); nc.sync.dma_start(b1x0[:], din["b1"][0:128])
        b1x1 = const.tile([DT1, 1], f32); nc.sync.dma_start(b1x1[:], din["b1"][128:192])
        b1z0 = const.tile([DT0, 1], f32); nc.sync.dma_start(b1z0[:], din["b1"][192:320])
        b1z1 = const.tile([DT1, 1], f32); nc.sync.dma_start(b1z1[:], din["b1"][320:384])
        cdiag0 = const.tile([DT0, 9, DT0], bf16)
        nc.sync.dma_start(cdiag0[:], din["cdiag0"][:])
        cdiag1 = const.tile([DT1, 9, DT1], bf16)
        nc.sync.dma_start(cdiag1[:], din["cdiag1"][:])
        convb0, convb1 = load2("convb", DIN, 1)
        cdiag1d = const.tile([DT1, 9, 128], bf16)
        nc.sync.dma_start(cdiag1d[:], din["cdiag1d"][:])
        cbd = const.tile([128, 1], f32)
        nc.sync.dma_start(cbd[:], din["cbd"][:])
        red1 = const.tile([128, DT1], bf16)
        nc.sync.dma_start(red1[:], din["red1"][:])
        eye = const.tile([128, 128], bf16)
        nc.sync.dma_start(eye[:], din["eye"][:])
        dsum0, dsum1 = load2("dsum", DIN, 1)
        mrow0, mrow1 = load2("mrow", DIN, 1)
        mcol0, mcol1 = load2("mcol", DIN, 1)
        kw = []
        for k in range(2):
            xp0 = const.tile([DT0, RNK + 2 * NST], bf16, name=f"xp{k}0")
            xp1 = const.tile([128, RNK + 2 * NST], bf16, name=f"xp{k}1")
            nc.sync.dma_start(xp0[:], din["xpw"][k, 0:DT0])
            nc.sync.dma_start(xp1[:], din["xpz"][k])
            dtw = const.tile([38, DIN], bf16, tag="dtwm", name=f"dtw{k}",
                             bufs=1) if k == 0 else kw[0]["dtwt"]
            nc.sync.dma_start(dtw[k * 32:k * 32 + RNK], din["dtw"][k])
            dtwd = const.tile([38, 128], bf16, tag="dtwdm", name=f"dtwd{k}",
                              bufs=1) if k == 0 else kw[0]["dtwdt"]
            nc.sync.dma_start(dtwd[k * 32:k * 32 + RNK], din["dtwd"][k])
            dtb0 = const.tile([DT0, 1], f32, name=f"dtb{k}0")
            dtb1 = const.tile([128, 1], f32, name=f"dtb{k}1")
            nc.sync.dma_start(dtb0[:], din["dtb"][k, 0:DT0])
            nc.sync.dma_start(dtb1[:], din["dtbd"][k])
            ac0 = const.tile([DT0, NST], f32, name=f"ac{k}0")
            ac1 = const.tile([128, 8], f32, name=f"ac{k}1")
            nc.sync.dma_start(ac0[:], din["acoef"][k, 0:DT0])
            nc.sync.dma_start(ac1[:], din["acp"][k])
            kw.append(dict(xp=(xp0, xp1), dtwt=dtw, dtwdt=dtwd,
                           dtw=dtw[k * 32:k * 32 + RNK],
                           dtwd=dtwd[k * 32:k * 32 + RNK],
                           dtb=(dtb0, dtb1),
                           ac=(ac0, ac1)))

        ones128 = const.tile([128, 1], f32); nc.vector.memset(ones128[:], 1.0)
        onesrow = const.tile([1, 128], bf16); nc.vector.memset(onesrow[:], 1.0)
        epsc = const.tile([1, 1], f32); nc.vector.memset(epsc[:], EPS)

        # ---- proj (f32r matmuls, x_t kept fp32 for residual) ----
        x_t = big.tile([COUT, L], f32, tag="x_t")
        for (s, w) in MM:
            ps = psM.tile([128, MMC], f32, tag="mm", name="psproj")
            nc.tensor.matmul(ps[:COUT, :w], projW0[:], xc0[:, s:s + w],
                             start=True, stop=False)
            nc.tensor.matmul(ps[:COUT, :w], projW1[:], xc1[:, s:s + w],
                             start=False, stop=True)
            nc.scalar.activation(x_t[:, s:s + w], ps[:COUT, :w], AF.Identity,
                                 bias=projb[:])
        nc.sync.dma_start(ox_d[:], x_t[:])

        # ---- LN1 (Copy + Sqrt share the act-table phase) -> xn bf16 ----
        xn_t = big.tile([COUT, L], bf16, tag="xn")
        for (s, w) in MM:
            ps1 = psM.tile([128, MMC], f32, tag="mm", name="pss1")
            nc.tensor.matmul(ps1[:1, :w], ones128[:COUT], x_t[:, s:s + w],
                             start=True, stop=True)
            mrw = work.tile([1, MMC], bf16, tag="mrw", bufs=1)
            nc.scalar.activation(mrw[:, :w], ps1[:1, :w], AF.Copy, scale=1.0 / COUT)
            sq = work.tile([128, MMC], f32, tag="sqc", bufs=1)
            nc.vector.tensor_tensor(out=sq[:COUT, :w], in0=x_t[:, s:s + w],
                                    in1=x_t[:, s:s + w], op=OP_.mult)
            ps2 = psM.tile([128, MMC], f32, tag="mm", name="pss2")
            nc.tensor.matmul(ps2[:1, :w], ones128[:COUT], sq[:COUT, :w],
                             start=True, stop=True)
            mq = work.tile([1, MMC], f32, tag="mq", bufs=1)
            nc.scalar.activation(mq[:, :w], ps2[:1, :w], AF.Copy, scale=1.0 / COUT)
            msq = work.tile([1, MMC], f32, tag="msq", bufs=1)
            nc.vector.tensor_tensor(out=msq[:, :w], in0=mrw[:, :w],
                                    in1=mrw[:, :w], op=OP_.mult)
            nc.vector.tensor_tensor(out=mq[:, :w], in0=mq[:, :w],
                                    in1=msq[:, :w], op=OP_.subtract)
            nc.scalar.activation(mq[:, :w], mq[:, :w], AF.Sqrt, bias=epsc[:])
            rsw = work.tile([1, MMC], bf16, tag="rsw", bufs=1)
            with nc.allow_low_precision(reason="bf16 1/std is well conditioned"):
                nc.vector.reciprocal(rsw[:, :w], mq[:, :w])
            pm = psM.tile([128, MMC], f32, tag="mm", name="psbm")
            nc.tensor.matmul(pm[:, :w], onesrow[:], mrw[:, :w],
                             start=True, stop=True)
            pr = psM.tile([128, MMC], f32, tag="mm", name="psbr")
            nc.tensor.matmul(pr[:, :w], onesrow[:], rsw[:, :w],
                             start=True, stop=True)
            xn_ = work.tile([128, MMC], bf16, tag="xn_", bufs=1)
            nc.vector.tensor_tensor(out=xn_[:COUT, :w], in0=x_t[:, s:s + w],
                                    in1=pm[:COUT, :w], op=OP_.subtract)
            nc.vector.tensor_tensor(out=xn_t[:, s:s + w], in0=xn_[:COUT, :w],
                                    in1=pr[:COUT, :w], op=OP_.mult)

        # ---- in_proj (xm tiles bf16; z silu'd -> DRAM bf16) ----
        xm0 = big.tile([DT0, L], bf16, tag="xm0")
        xm1 = big.tile([DT1, L], bf16, tag="xm1")
        for (s, w) in MM:
            for (coff, rows, bcol, dst, zoff) in (
                    (0, DT0, b1x0, xm0, None), (DT0, DT1, b1x1, xm1, None),
                    (DIN, DT0, b1z0, None, 0), (DIN + DT0, DT1, b1z1, None, DT0)):
                psi = psM.tile([128, MMC], f32, tag="mm", name="psip")
                nc.tensor.matmul(psi[:rows, :w], W1t[:, coff:coff + rows],
                                 xn_t[:, s:s + w], start=True, stop=True)
                if dst is not None:
                    nc.scalar.activation(dst[:, s:s + w], psi[:rows, :w],
                                         AF.Identity, bias=bcol[:])
                else:
                    zc = work.tile([128, MMC], bf16, tag="zc", bufs=1)
                    nc.scalar.activation(zc[:rows, :w], psi[:rows, :w], AF.Silu,
                                         bias=bcol[:])
                    nc.sync.dma_start(oz_d[zoff:zoff + rows, s:s + w], zc[:rows, :w])

        # ---- depthwise conv via PE diag matmuls + fused SiLU; the
        #      transposed copy for the plane transform happens per row-chunk
        xs0 = big.tile([DT0, L], bf16, tag="xs0")
        xs1 = big.tile([128, L], bf16, tag="xs1")
        xt0 = big.tile([DT0, L], bf16, tag="xm0", name="xt0")
        xt1 = big.tile([128, L], bf16, tag="xm1", name="xt1")
        for (src, cd, srows, orows, out, bcol, xtt, teng) in (
                (xm0, cdiag0, DT0, DT0, xs0, convb0, xt0, nc.vector),
                (xm1, cdiag1d, DT1, 128, xs1, cbd, xt1, nc.vector)):
            pad = big.tile([128, 50, 50], bf16, tag="xc0", name="pad")
            nc.vector.memset(pad[:srows], 0.0)
            r0 = 0
            for (s, w) in MM:
                r1 = (s + w) // 48
                nc.vector.tensor_copy(out=pad[:srows, 1 + r0:1 + r1, 1:49],
                                      in_=_pl3(src[:])[:, r0:r1, :])
                r0 = r1
            for (y0, ny) in CROWS:
                pc = psM.tile([128, MMC], f32, tag="mm", name="pscv")
                for j in range(9):
                    dy, dx = divmod(j, 3)
                    view = pad[:srows, y0 + dy:y0 + dy + ny, dx:dx + 48]
                    nc.tensor.matmul(pc[:orows, :ny * 48], cd[:, j], view,
                                     start=(j == 0), stop=(j == 8))
                nc.scalar.activation(out[:, y0 * 48:(y0 + ny) * 48],
                                     pc[:orows, :ny * 48], AF.Silu, bias=bcol[:])
                # xt[p, x, y] = xs[p, y, x] for this y-chunk
                xin = _pl3(out[:])[:, y0:y0 + ny, :]
                xout = bass.AP(tensor=xtt.tensor, offset=xtt[:].offset + y0,
                               ap=[xtt[:].ap[0], [1, ny], [48, 48]])
                teng.tensor_copy(out=xout, in_=xin)
        nc.vector.tensor_scalar_mul(xs0[:], xs0[:], mrow0[:])
        nc.vector.scalar_tensor_tensor(out=xs0[:], in0=xt0[:], scalar=mcol0[:],
                                       in1=xs0[:], op0=OP_.mult, op1=OP_.add)
        nc.vector.tensor_scalar_mul(xs1[:], xs1[:], mrow0[:])
        nc.vector.scalar_tensor_tensor(out=xs1[:], in0=xt1[:], scalar=mcol0[:],
                                       in1=xs1[:], op0=OP_.mult, op1=OP_.add)

        # ---- U96: x_dbl for both k; B/C staged to DRAM bf16 ----
        u96m = big.tile([38, L], bf16, tag="u96r")
        u96rs = (u96m[0:RNK], u96m[32:38])
        for k in range(2):
            W = kw[k]
            rb = k * 32          # rank section base: 0 (k0) / 32 (k1)
            bb = rb + 32
            cb = 64 if k == 0 else 0
            for (s, w) in MM:
                ps = psM.tile([128, MMC], f32, tag="mm", name="psU")
                for (coff, ubase, m) in ((0, rb, RNK), (RNK, bb, NST),
                                         (RNK + NST, cb, NST)):
                    nc.tensor.matmul(ps[ubase:ubase + m, :w],
                                     W["xp"][0][:, coff:coff + m],
                                     xs0[:, s:s + w], start=True, stop=False)
                    nc.tensor.matmul(ps[ubase:ubase + m, :w],
                                     W["xp"][1][:, coff:coff + m],
                                     xs1[:, s:s + w], start=False, stop=True)
                nc.vector.tensor_copy(out=u96rs[k][:, s:s + w],
                                       in_=ps[rb:rb + RNK, :w])
                bcsw = work.tile([112, MMC], bf16, tag="bcsw")
                nc.vector.tensor_copy(out=bcsw[bb:bb + NST, :w],
                                      in_=ps[bb:bb + NST, :w])
                nc.vector.tensor_copy(out=bcsw[cb:cb + NST, :w],
                                      in_=ps[cb:cb + NST, :w])
                nc.sync.dma_start(bcd[k, 0:NST, s:s + w], bcsw[bb:bb + NST, :w])
                nc.sync.dma_start(bcd[k, NST:32, s:s + w], bcsw[cb:cb + NST, :w])

        # ---- delta (Softplus) for both k, both dt ----
        dcs = []
        for k in range(2):
            W = kw[k]
            dc0 = big.tile([DT0, L], bf16, tag="xc0" if k == 0 else "xc1",
                           name=f"dc{k}0")
            dc1 = big.tile([128, L], bf16, tag=f"dc{k}1", name=f"dc{k}1")
            for (s, w) in MM:
                for dt, (rows, stat, dct) in enumerate(
                        ((DT0, W["dtw"][:, 0:DT0], dc0),
                         (128, W["dtwd"], dc1))):
                    ps = psM.tile([128, MMC], f32, tag="mm", name="psdt")
                    nc.tensor.matmul(ps[:rows, :w], stat,
                                     u96rs[k][:, s:s + w],
                                     start=True, stop=True)
                    nc.scalar.activation(dct[:, s:s + w], ps[:rows, :w],
                                         AF.Exp, bias=W["dtb"][dt][:])
            # in-place Ln (softplus) + dxc = delta*x, per chunk
            dx0 = big.tile([DT0, L], bf16, tag="x_t" if k == 0 else "xn",
                           name=f"dx{k}0")
            dx1 = big.tile([128, L], bf16, tag="xm1" if k == 0 else "u96r",
                           name=f"dx{k}1")
            for (s, w) in MM:
                for dct, dxt, xst in ((dc0, dx0, xs0), (dc1, dx1, xs1)):
                    nc.scalar.activation(dct[:, s:s + w], dct[:, s:s + w],
                                         AF.Ln, bias=1.0)
                    nc.vector.tensor_tensor(out=dxt[:, s:s + w],
                                            in0=dct[:, s:s + w],
                                            in1=xst[:, s:s + w], op=OP_.mult)
            dcs.append((dc0, dc1, dx0, dx1))

        # ---- P init with the direction-independent D term ----
        P0 = big.tile([DT0, L], bf16, tag="xm0", name="P0")
        P1 = big.tile([DT1, L], bf16, tag="xt1m", name="P1")
        nc.vector.tensor_scalar_mul(P0[:], xs0[:], dsum0[:])
        nc.gpsimd.tensor_scalar_mul(P1[:], xs1[0:DT1], dsum1[:])

        # ---- selective scan ----
        hp = [[scn.tile([128, NST if dt == 0 else 8, 1], bf16,
                        tag=f"hp{k}{dt}", name=f"hp{k}{dt}")
               for dt in range(2)] for k in range(2)]
        for k in range(2):
            for dt in range(2):
                nc.vector.memset(hp[k][dt][:], 0.0)

        pairs = [(c, k) for c in range(len(SC)) for k in range(2)]
        bcr_tiles = {}

        def issue_bcast(i):
            c, k = pairs[i]
            s, w = SC[c]
            s0 = s if k == 0 else L - s - w
            t = scn.tile([128, 40, LC], bf16, tag="bcr", bufs=3,
                         name=f"bcr{i}")
            src = bass.AP(tensor=bcd, offset=k * 32 * L + s0,
                          ap=[[0, 128], [L, 32], [1, w]])
            nc.sync.dma_start(t[:, 0:32], src)
            # packed dt1 C': rows 0:64 get states 16:24, rows 64:128 get 24:32
            for (p0, srow) in ((0, 16), (64, 24)):
                sp = bass.AP(tensor=bcd, offset=k * 32 * L + srow * L + s0,
                             ap=[[0, 64], [L, 8], [1, w]])
                nc.sync.dma_start(t[p0:p0 + 64, 32:40, :], sp)
            bcr_tiles[i] = t

        pendingP = []

        def tail(dt, dA, Ht, crep, hpt, pdst):
            # after the scan: save carry, G = H*C, PE n-reduction; the P
            # accumulate is deferred further so DVE never waits on the PE
            if len(pendingP) >= 2:
                pendingP.pop(0)()
            nc.gpsimd.tensor_copy(out=hpt[:], in_=Ht[:, :, LC:LC + 1])
            psy = psY.tile([128, LC], f32, tag="psy", name="psy", bufs=3)
            if dt == 0:
                nc.vector.tensor_tensor(out=dA[:, 0:12, 1:],
                                        in0=Ht[:, 0:12, 1:],
                                        in1=crep[:, 0:12], op=OP_.mult)
                nc.gpsimd.tensor_tensor(out=dA[:, 12:NST, 1:],
                                        in0=Ht[:, 12:NST, 1:],
                                        in1=crep[:, 12:NST], op=OP_.mult)
                for n in range(NST):
                    nc.tensor.matmul(psy[:, :], eye[:, :],
                                     dA[:, n, 1:],
                                     start=(n == 0), stop=(n == NST - 1))
                rows = DT0
            else:
                nc.vector.tensor_tensor(out=dA[:, :, 1:],
                                        in0=Ht[:, :, 1:],
                                        in1=crep, op=OP_.mult)
                for n in range(8):
                    nc.tensor.matmul(psy[:DT1, :], red1[:, :],
                                     dA[:, n, 1:],
                                     start=(n == 0), stop=(n == 7))
                rows = DT1

            def paccum(psy=psy, rows=rows, pdst=pdst):
                nc.vector.tensor_tensor(out=pdst, in0=psy[:rows, :], in1=pdst,
                                        op=OP_.add)
            pendingP.append(paccum)

        pending = []
        issue_bcast(0)
        for i, (c, k) in enumerate(pairs):
            s, w = SC[c]
            W = kw[k]
            bcr = bcr_tiles.pop(i)
            if k == 0:
                brep = bcr[:, 0:NST, :]
                crep = bcr[:, NST:32, :]
            else:
                brep = bcr[:, 0:NST, ::-1]
                crep = bcr[:, NST:32, ::-1]
            for dt in range(2):
                ns = NST if dt == 0 else 8
                if len(pending) >= 3:
                    # flush the 3-iterations-old deferred tail before its
                    # dA/dBu/Ht buffer slots are reused below
                    pending.pop(0)()
                if dt == 0 and i + 1 < len(pairs):
                    # safe point: all readers of bcr slot (i+1)%3's previous
                    # occupant (pair i-2) have been emitted by now
                    issue_bcast(i + 1)
                dct = dcs[k][dt]
                dcsl = _sl(dct[:], k, s, w)
                dxsl = _sl(dcs[k][2 + dt][:], k, s, w)
                dA = scn.tile([128, ns, LC + 1], bf16, tag="dA",
                              name=f"dA{dt}", bufs=3)
                dBu = scn.tile([128, ns, LC + 1], bf16, tag="dBu",
                               name=f"dBu{dt}", bufs=2)
                Ht = scn.tile([128, ns, LC + 1], bf16, tag="Ht",
                              name=f"Ht{dt}", bufs=3)
                nc.gpsimd.memset(dA[:, :, 0:1], 0.0)
                nc.gpsimd.tensor_copy(out=dBu[:, :, 0:1], in_=hp[k][dt][:])
                for n in range(ns):
                    nc.scalar.activation(dA[:, n, 1:], dcsl, AF.Exp,
                                         scale=W["ac"][dt][:, n:n + 1])
                beng = nc.vector if dt == 0 else nc.gpsimd
                if dt == 0:
                    beng.tensor_tensor(out=dBu[:, :, 1:],
                                       in0=_rep(dxsl, NST),
                                       in1=brep, op=OP_.mult)
                else:
                    # packed: nh half selects B rows 0:8 / 8:16
                    beng.tensor_tensor(out=dBu[0:DT1, :, 1:],
                                       in0=_rep(dxsl[0:DT1], 8),
                                       in1=brep[0:DT1, 0:8], op=OP_.mult)
                    beng.tensor_tensor(out=dBu[DT1:128, :, 1:],
                                       in0=_rep(dxsl[DT1:128], 8),
                                       in1=brep[DT1:128, 8:NST], op=OP_.mult)
                nc.vector.tensor_tensor_scan(
                    out=Ht[:].rearrange("p a b -> p (a b)"),
                    data0=dA[:].rearrange("p a b -> p (a b)"),
                    data1=dBu[:].rearrange("p a b -> p (a b)"),
                    initial=0.0, op0=OP_.mult, op1=OP_.add)
                Pt = P0 if dt == 0 else P1
                cr = crep if dt == 0 else (
                    bcr[:, 32:40, :] if k == 0 else bcr[:, 32:40, ::-1])
                args = (dt, dA, Ht, cr, hp[k][dt], _sl(Pt[:], k, s, w))
                pending.append(lambda a=args: tail(*a))
        for fn in pending:
            fn()
        for fn in pendingP:
            fn()

        # ---- Q = mrow*P + mcol*transpose(P) ----
        Q0 = big.tile([DT0, L], bf16, tag="xs0", name="Q0")
        Q1 = big.tile([DT1, L], bf16, tag="xs1", name="Q1")
        nc.vector.tensor_scalar_mul(Q0[:], _twh(P0[:]), mcol0[:])
        nc.vector.scalar_tensor_tensor(out=Q0[:], in0=P0[:], scalar=mrow0[:],
                                       in1=Q0[:], op0=OP_.mult, op1=OP_.add)
        nc.gpsimd.tensor_scalar_mul(Q1[:], _twh(P1[:]), mcol1[:])
        qtm = big.tile([DT1, L], bf16, tag="xm1", name="qtm")
        nc.gpsimd.tensor_scalar_mul(qtm[:], P1[:], mrow1[:])
        nc.gpsimd.tensor_tensor(out=Q1[:], in0=qtm[:], in1=Q1[:], op=OP_.add)
        nc.sync.dma_start(oq_d[0:DT0], Q0[:])
        nc.sync.dma_start(oq_d[DT0:DIN], Q1[:])
    nc.compile()
    return nc


# ---------------------------------------------------------------- pass 2
def build_nc2():
    nc = bacc.Bacc("TRN2", target_bir_lowering=False, debug=False, num_devices=8)
    din = {}

    def I(name, shape, dt=f32):
        din[name] = nc.dram_tensor(name, shape, dt, kind="ExternalInput")

    I("ym", [DIN, L], bf16); I("xin", [COUT, L]); I("zin", [DIN, L], bf16)
    I("OPm", [DIN, COUT], bf16); I("OPB", [DIN, COUT], bf16)
    I("PW1", [COUT, HID], bf16); I("g1", [HID, 1]); I("bb1", [HID, 1])
    I("cbdiag0", [DT0, 9, DT0], bf16); I("cbdiag1", [DT1, 9, DT1], bf16)
    I("g2", [HID, 1]); I("bb2", [HID, 1])
    I("PW2", [HID, COUT], bf16); I("g3", [COUT, 1]); I("bb3", [COUT, 1])
    I("fw", [COUT, 1]); I("fb", [COUT, 1])
    out_d = nc.dram_tensor("o", [COUT, L], f32, kind="ExternalOutput")

    ctx = contextlib.ExitStack()
    with tile.TileContext(nc) as tc, ctx:
        const = ctx.enter_context(tc.tile_pool(name="const", bufs=1))
        big = ctx.enter_context(tc.tile_pool(name="big", bufs=1))
        work = ctx.enter_context(tc.tile_pool(name="work", bufs=2))
        psM = ctx.enter_context(tc.tile_pool(name="psM", bufs=2, space="PSUM"))

        def load2(name, rows, cols, dt=f32):
            t0 = const.tile([DT0, cols], dt, tag=name + "0", name=name + "0")
            t1 = const.tile([DT1, cols], dt, tag=name + "1", name=name + "1")
            nc.sync.dma_start(t0[:], din[name][0:DT0])
            nc.sync.dma_start(t1[:], din[name][DT0:rows])
            return t0, t1

        def load1(name, rows):
            t = const.tile([rows, 1], f32, tag=name, name=name)
            nc.sync.dma_start(t[:], din[name][:])
            return t

        # input data first so the out-norm chain isn't stuck behind consts
        ym0 = big.tile([DT0, L], bf16, tag="ym0")
        ym1 = big.tile([DT1, L], bf16, tag="ym1")
        xres = big.tile([COUT, L], f32, tag="xres")
        zc0 = big.tile([DT0, L], bf16, tag="zc0")
        zc1 = big.tile([DT1, L], bf16, tag="zc1")
        for (s, w) in MM:
            nc.sync.dma_start(ym0[:, s:s + w], din["ym"][0:DT0, s:s + w])
            nc.sync.dma_start(ym1[:, s:s + w], din["ym"][DT0:DIN, s:s + w])
            nc.sync.dma_start(zc0[:, s:s + w], din["zin"][0:DT0, s:s + w])
            nc.sync.dma_start(zc1[:, s:s + w], din["zin"][DT0:DIN, s:s + w])
            nc.sync.dma_start(xres[:, s:s + w], din["xin"][:, s:s + w])
        OP0, OP1 = load2("OPm", DIN, COUT, bf16)
        OPB0, OPB1 = load2("OPB", DIN, COUT, bf16)
        PW1t = const.tile([COUT, HID], bf16)
        nc.sync.dma_start(PW1t[:], din["PW1"][:])
        g1c0, g1c1 = load2("g1", HID, 1)
        bb1c0, bb1c1 = load2("bb1", HID, 1)
        cbd0 = const.tile([DT0, 9, DT0], bf16)
        nc.sync.dma_start(cbd0[:], din["cbdiag0"][:])
        cbd1 = const.tile([DT1, 9, DT1], bf16)
        nc.sync.dma_start(cbd1[:], din["cbdiag1"][:])
        g2c0, g2c1 = load2("g2", HID, 1)
        bb2c0, bb2c1 = load2("bb2", HID, 1)
        PW20, PW21 = load2("PW2", HID, COUT, bf16)
        g3c = load1("g3", COUT); bb3c = load1("bb3", COUT)
        fwc = load1("fw", COUT); fbc = load1("fb", COUT)
        onesb = const.tile([128, 1], bf16); nc.vector.memset(onesb[:], 1.0)
        onesrow = const.tile([1, 128], bf16); nc.vector.memset(onesrow[:], 1.0)
        epsc = const.tile([1, 1], f32); nc.vector.memset(epsc[:], EPS)

        # out-norm stats over 192 partitions (per-chunk)
        mean_r = big.tile([1, L], bf16, tag="mean")
        rs_r = big.tile([1, L], bf16, tag="rs")
        for (s, w) in MM:
            ps = psM.tile([128, MMC], f32, tag="mm", name="pso1")
            nc.tensor.matmul(ps[:1, :w], onesb[:], ym0[:, s:s + w],
                             start=True, stop=False)
            nc.tensor.matmul(ps[:1, :w], onesb[:DT1], ym1[:, s:s + w],
                             start=False, stop=True)
            nc.scalar.activation(mean_r[:, s:s + w], ps[:1, :w], AF.Copy,
                                 scale=1.0 / DIN)
            ps2 = psM.tile([128, MMC], f32, tag="mm", name="pso2")
            for i, (t, rows) in enumerate(((ym0, DT0), (ym1, DT1))):
                sq = work.tile([128, MMC], bf16, tag="sqc", bufs=1)
                nc.vector.tensor_tensor(out=sq[:rows, :w], in0=t[:, s:s + w],
                                        in1=t[:, s:s + w], op=OP_.mult)
                nc.tensor.matmul(ps2[:1, :w], onesb[:rows], sq[:rows, :w],
                                 start=(i == 0), stop=(i == 1))
            mq = work.tile([1, MMC], f32, tag="mq", bufs=1)
            nc.scalar.activation(mq[:, :w], ps2[:1, :w], AF.Copy,
                                 scale=1.0 / DIN)
            msqc = work.tile([1, MMC], f32, tag="msqc", bufs=1)
            nc.vector.tensor_tensor(out=msqc[:, :w], in0=mean_r[:, s:s + w],
                                    in1=mean_r[:, s:s + w], op=OP_.mult)
            nc.vector.tensor_tensor(out=mq[:, :w], in0=mq[:, :w],
                                    in1=msqc[:, :w], op=OP_.subtract)
            nc.scalar.activation(mq[:, :w], mq[:, :w], AF.Sqrt, bias=epsc[:])
            with nc.allow_low_precision(reason="bf16 1/std is well conditioned"):
                nc.vector.reciprocal(rs_r[:, s:s + w], mq[:, :w])

        x2f = big.tile([COUT, L], f32, tag="x2f")
        x2b = big.tile([COUT, L], bf16, tag="x2b")
        for (s, w) in MM:
            pm = psM.tile([128, MMC], f32, tag="mm", name="psm")
            nc.tensor.matmul(pm[:, :w], onesrow[:], mean_r[:, s:s + w],
                             start=True, stop=True)
            pr = psM.tile([128, MMC], f32, tag="mm", name="psr")
            nc.tensor.matmul(pr[:, :w], onesrow[:], rs_r[:, s:s + w],
                             start=True, stop=True)
            po = psM.tile([128, MMC], f32, tag="mm", name="pso")
            for i, (t, z, rows) in enumerate(((ym0, zc0, DT0), (ym1, zc1, DT1))):
                yn = work.tile([128, MMC], bf16, tag=f"yn{i}", name=f"yn{i}")
                nc.vector.tensor_tensor(out=yn[:rows, :w], in0=t[:, s:s + w],
                                        in1=pm[:rows, :w], op=OP_.subtract)
                nc.vector.tensor_tensor(out=yn[:rows, :w], in0=yn[:rows, :w],
                                        in1=pr[:rows, :w], op=OP_.mult)
                nc.vector.tensor_tensor(out=yn[:rows, :w], in0=yn[:rows, :w],
                                        in1=z[:, s:s + w], op=OP_.mult)
                OPt = OP0 if i == 0 else OP1
                OPBt = OPB0 if i == 0 else OPB1
                nc.tensor.matmul(po[:COUT, :w], OPt[:], yn[:rows, :w],
                                 start=(i == 0), stop=False)
                nc.tensor.matmul(po[:COUT, :w], OPBt[:], z[:, s:s + w],
                                 start=False, stop=(i == 1))
            nc.vector.tensor_tensor(out=x2f[:, s:s + w], in0=po[:COUT, :w],
                                    in1=xres[:, s:s + w], op=OP_.add)
            nc.scalar.activation(x2b[:, s:s + w], x2f[:, s:s + w], AF.Copy)

        # ConvBlock: PW1 + gelu
        t0 = big.tile([DT0, L], bf16, tag="ym0", name="t0")
        t1 = big.tile([DT1, L], bf16, tag="ym1", name="t1")
        for (s, w) in MM:
            for (dst, coff, rows, gc_, bc_) in ((t0, 0, DT0, g1c0, bb1c0),
                                                (t1, DT0, DT1, g1c1, bb1c1)):
                ps = psM.tile([128, MMC], f32, tag="mm", name="psp1")
                nc.tensor.matmul(ps[:rows, :w], PW1t[:, coff:coff + rows],
                                 x2b[:, s:s + w], start=True, stop=True)
                nc.scalar.activation(dst[:, s:s + w], ps[:rows, :w], AF.Gelu,
                                     bias=bc_[:], scale=gc_[:])
        # dw conv via PE; fused bn2+gelu on psum
        v0 = big.tile([DT0, L], bf16, tag="zc0", name="v0")
        v1 = big.tile([DT1, L], bf16, tag="zc1", name="v1")
        for (src, cd, rows, out, gc_, bc_) in (
                (t0, cbd0, DT0, v0, g2c0, bb2c0),
                (t1, cbd1, DT1, v1, g2c1, bb2c1)):
            pad = work.tile([128, 50, 50], bf16, tag="pad", bufs=1)
            nc.vector.memset(pad[:rows], 0.0)
            r0 = 0
            for (s, w) in MM:
                r1 = (s + w) // 48
                nc.vector.tensor_copy(out=pad[:rows, 1 + r0:1 + r1, 1:49],
                                      in_=_pl3(src[:])[:, r0:r1, :])
                r0 = r1
            for (y0, ny) in CROWS:
                pc = psM.tile([128, MMC], f32, tag="mm", name="pscv")
                for j in range(9):
                    dy, dx = divmod(j, 3)
                    view = pad[:rows, y0 + dy:y0 + dy + ny, dx:dx + 48]
                    nc.tensor.matmul(pc[:rows, :ny * 48], cd[:, j], view,
                                     start=(j == 0), stop=(j == 8))
                nc.scalar.activation(out[:, y0 * 48:(y0 + ny) * 48],
                                     pc[:rows, :ny * 48], AF.Gelu,
                                     bias=bc_[:], scale=gc_[:])
        # PW2 + bn3 + residual
        x3f = big.tile([COUT, L], f32, tag="x3f")
        x3b = big.tile([COUT, L], bf16, tag="xres", name="x3b")
        for (s, w) in MM:
            ps = psM.tile([128, MMC], f32, tag="mm", name="psp2")
            nc.tensor.matmul(ps[:COUT, :w], PW20[:], v0[:, s:s + w],
                             start=True, stop=False)
            nc.tensor.matmul(ps[:COUT, :w], PW21[:], v1[:, s:s + w],
                             start=False, stop=True)
            cbt = work.tile([128, MMC], bf16, tag="cbt", bufs=1)
            nc.scalar.activation(cbt[:COUT, :w], ps[:COUT, :w], AF.Identity,
                                 bias=bb3c[:], scale=g3c[:])
            nc.vector.tensor_tensor(out=x3f[:, s:s + w], in0=cbt[:COUT, :w],
                                    in1=x2f[:, s:s + w], op=OP_.add)
            nc.scalar.activation(x3b[:, s:s + w], x3f[:, s:s + w], AF.Copy)

        # final LN
        mean2 = big.tile([1, L], bf16, tag="mean2")
        rs2 = big.tile([1, L], bf16, tag="rs2")
        for (s, w) in MM:
            ps = psM.tile([128, MMC], f32, tag="mm", name="psf1")
            nc.tensor.matmul(ps[:1, :w], onesb[:COUT], x3b[:, s:s + w],
                             start=True, stop=True)
            nc.scalar.activation(mean2[:, s:s + w], ps[:1, :w], AF.Copy,
                                 scale=1.0 / COUT)
            sq = work.tile([128, MMC], bf16, tag="sqc", bufs=1)
            nc.vector.tensor_tensor(out=sq[:COUT, :w], in0=x3b[:, s:s + w],
                                    in1=x3b[:, s:s + w], op=OP_.mult)
            ps2 = psM.tile([128, MMC], f32, tag="mm", name="psf2")
            nc.tensor.matmul(ps2[:1, :w], onesb[:COUT], sq[:COUT, :w],
                             start=True, stop=True)
            mq2 = work.tile([1, MMC], f32, tag="mq2", bufs=1)
            nc.scalar.activation(mq2[:, :w], ps2[:1, :w], AF.Copy,
                                 scale=1.0 / COUT)
            msqc2 = work.tile([1, MMC], f32, tag="msqc2", bufs=1)
            nc.vector.tensor_tensor(out=msqc2[:, :w], in0=mean2[:, s:s + w],
                                    in1=mean2[:, s:s + w], op=OP_.mult)
            nc.vector.tensor_tensor(out=mq2[:, :w], in0=mq2[:, :w],
                                    in1=msqc2[:, :w], op=OP_.subtract)
            nc.scalar.activation(mq2[:, :w], mq2[:, :w], AF.Sqrt, bias=epsc[:])
            with nc.allow_low_precision(reason="bf16 1/std is well conditioned"):
                nc.vector.reciprocal(rs2[:, s:s + w], mq2[:, :w])
        for (s, w) in MM:
            pm = psM.tile([128, MMC], f32, tag="mm", name="psfm")
            nc.tensor.matmul(pm[:, :w], onesrow[:], mean2[:, s:s + w],
                             start=True, stop=True)
            pr = psM.tile([128, MMC], f32, tag="mm", name="psfr")
            nc.tensor.matmul(pr[:, :w], onesrow[:], rs2[:, s:s + w],
                             start=True, stop=True)
            oc = work.tile([128, MMC], f32, tag="oc", bufs=1)
            nc.vector.tensor_tensor(out=oc[:COUT, :w], in0=x3f[:, s:s + w],
                                    in1=pm[:COUT, :w], op=OP_.subtract)
            nc.vector.tensor_tensor(out=oc[:COUT, :w], in0=oc[:COUT, :w],
                                    in1=pr[:COUT, :w], op=OP_.mult)
            nc.vector.tensor_scalar(out=oc[:COUT, :w], in0=oc[:COUT, :w],
                                    scalar1=fwc[:], scalar2=fbc[:],
                                    op0=OP_.mult, op1=OP_.add)
            nc.sync.dma_start(out_d[:, s:s + w], oc[:COUT, :w])
    nc.compile()
    return nc


_NC1, _NC2 = None, None


def _get_ncs():
    global _NC1, _NC2
    if _NC1 is None:
        _NC1 = build_nc1()
        _NC2 = build_nc2()
    return _NC1, _NC2


def _bf(a):
    import jax.numpy as jnp
    return np.asarray(jnp.asarray(np.asarray(a, np.float32), jnp.bfloat16))


def _diag9(wmat, rows):
    out = np.zeros((rows, 9, rows), np.float32)
    idx = np.arange(rows)
    for j in range(9):
        out[idx, j, idx] = wmat[:, j]
    return out


def prep_pass1(ip):
    W1 = (np.diag(ip["ln1_w"]) @ ip["in_proj_W"]).astype(np.float32)
    b1 = (ip["ln1_b"] @ ip["in_proj_W"] + ip["in_proj_b"]).astype(np.float32)
    A = (-np.exp(ip["A_logs"].astype(np.float64))).astype(np.float32).reshape(KDIR, DIN, NST)
    Ds = ip["Ds"].reshape(KDIR, DIN)
    col = lambda v: np.ascontiguousarray(v.reshape(-1, 1), dtype=np.float32)
    convW = ip["conv_W"].reshape(DIN, 9)
    base = dict(projW=_bf(ip["proj_W"]), projb=col(ip["proj_b"]), W1=_bf(W1),
                b1=col(b1),
                cdiag0=_bf(_diag9(convW[0:DT0], DT0)),
                cdiag1=_bf(_diag9(convW[DT0:DIN], DT1)),
                convb=col(ip["conv_b"]),
                eye=_bf(np.eye(128, dtype=np.float32)))
    # packed dt1 (channels 128:192 as p = d + 64*nh, 8 states per slot)
    cd1 = np.zeros((DT1, 9, 128), np.float32)
    di = np.arange(DT1)
    for j in range(9):
        cd1[di, j, di] = convW[DT0 + di, j]
        cd1[di, j, DT1 + di] = convW[DT0 + di, j]
    base["cdiag1d"] = _bf(cd1)
    base["cbd"] = col(np.tile(ip["conv_b"][DT0:], 2))
    base["red1"] = _bf(np.tile(np.eye(DT1, dtype=np.float32), (2, 1)))
    maps = []
    for c in range(8):
        b, plane = c // 2, c % 2
        ks = [plane, plane + 2]
        m = dict(base)
        m["xc_t"] = _bf(np.ascontiguousarray(ip["x_cat"][b].reshape(L, CIN).T))
        m["xpw"] = _bf(np.stack([ip["x_proj_W"][k].T for k in ks]))
        xpz = np.zeros((2, 128, RNK + 2 * NST), np.float32)
        for kk, k in enumerate(ks):
            xpz[kk, 0:DT1] = ip["x_proj_W"][k].T[DT0:DIN]
        m["xpz"] = _bf(xpz)
        m["dtw"] = _bf(np.stack([ip["dt_W"][k].T for k in ks]))
        m["dtwd"] = _bf(np.stack(
            [np.tile(ip["dt_W"][k].T[:, DT0:], (1, 2)) for k in ks]))
        m["dtb"] = np.ascontiguousarray(np.stack([col(ip["dt_b"][k]) for k in ks]))
        m["dtbd"] = np.ascontiguousarray(np.stack(
            [col(np.tile(ip["dt_b"][k][DT0:], 2)) for k in ks]))
        m["acoef"] = np.ascontiguousarray(np.stack([A[k] for k in ks]))
        acp = np.zeros((2, 128, 8), np.float32)
        for kk, k in enumerate(ks):
            for nh in range(2):
                acp[kk, nh * DT1:(nh + 1) * DT1, :] = A[k][DT0:DIN,
                                                           nh * 8:(nh + 1) * 8]
        m["acp"] = np.ascontiguousarray(acp)
        m["dsum"] = col(Ds[ks[0]] + Ds[ks[1]])
        m["mrow"] = np.full((DIN, 1), 1.0 - plane, np.float32)
        m["mcol"] = np.full((DIN, 1), float(plane), np.float32)
        maps.append(m)
    return maps


def prep_pass2(ip, res1):
    OPm = (np.diag(ip["out_norm_w"]) @ ip["out_proj_W"]).astype(np.float32)
    OPB = (np.diag(ip["out_norm_b"]) @ ip["out_proj_W"]).astype(np.float32)
    col = lambda v: np.ascontiguousarray(v.reshape(-1, 1), dtype=np.float32)
    cbw = ip["cb_dw_W"].reshape(HID, 9)
    base = dict(OPm=_bf(OPm), OPB=_bf(OPB),
                PW1=_bf(ip["cb_pw1_W"][:, :, 0, 0].T),
                g1=col(ip["cb_bn1_g"]), bb1=col(ip["cb_bn1_b"]),
                cbdiag0=_bf(_diag9(cbw[0:DT0], DT0)),
                cbdiag1=_bf(_diag9(cbw[DT0:HID], DT1)),
                g2=col(ip["cb_bn2_g"]), bb2=col(ip["cb_bn2_b"]),
                PW2=_bf(ip["cb_pw2_W"][:, :, 0, 0].T),
                g3=col(ip["cb_bn3_g"]), bb3=col(ip["cb_bn3_b"]),
                fw=col(ip["norm_w"]), fb=col(ip["norm_b"]))
    maps = []
    for c in range(8):
        b = c // 2
        m = dict(base)
        ymf = (np.asarray(res1[2 * b]["oq"], np.float32)
               + np.asarray(res1[2 * b + 1]["oq"], np.float32))
        m["ym"] = _bf(ymf)
        m["xin"] = np.asarray(res1[2 * b]["ox"], np.float32)
        m["zin"] = np.ascontiguousarray(res1[2 * b]["oz"])
        maps.append(m)
    return maps


def kernel(**inputs):
    ip = {k: np.asarray(v, np.float32) for k, v in inputs.items()}
    nc1, nc2 = _get_ncs()
    res1 = run_bass_kernel_spmd(nc1, prep_pass1(ip), list(range(8))).results
    res2 = run_bass_kernel_spmd(nc2, prep_pass2(ip, res1), list(range(8))).results
    outs = [np.asarray(res2[2 * b]["o"], np.float32).T.reshape(H_, W_, COUT)
            for b in range(B_)]
    return np.stack(outs).astype(np.float32)
